# revision 1
# baseline (speedup 1.0000x reference)
"""GCN block (GraphConv + BatchNorm1d + ReLU) on 8 Trainium2 NeuronCores.

Strategy (per sharding hint): partition nodes (and incident edges) across the
8 cores; replicate W/b/gamma/beta; all-reduce BN batch statistics.

Per core k (owns dst nodes [k*NPC, (k+1)*NPC)):
  1. h_k = (x_k @ W) * rsqrt(clip(deg_out_k,1))           (PE matmul, fp32)
  2. AllGather h (bf16) -> full h table in every core's HBM
  3. For each 128-node dst group, gather h[src] rows of the group's edges
     (dma_gather, bf16, batched) and segment-sum them with one-hot matmuls
     M^T @ G accumulated in PSUM (avoids dma_scatter_add, which loses
     updates on duplicate indices - verified on HW).
  4. relu(agg * rsqrt(clip(deg_in,1)) + b); local BN sums; AllReduce sums;
     y = (h - mu) * rsqrt(var+eps) * gamma + beta.

Host-side work is limited to integer index bookkeeping (bucketing edges by
(core, src-bank, dst-group), degree counting) and layout transforms (x^T,
int16 gather indices). All floating-point math runs on device.

Edges are bucketed by src bank (4 banks of N/4 rows) because dma_gather
indices are int16 (< 32768). Bucket sizes are padded to a structure shared
by all 8 cores so a single SPMD NEFF serves every core; pad slots gather row
0 of the bank and carry a dst offset of 255 -> their one-hot column is all
zeros, so they contribute exactly 0.
"""
import math
import os
import sys

sys.path.insert(0, "/opt/trn_rl_repo")

import numpy as np

import concourse.bacc as bacc
import concourse.bass as bass
import concourse.mybir as mybir
import concourse.tile as tile
from concourse import bass_utils

F32 = mybir.dt.float32
BF16 = mybir.dt.bfloat16
I16 = mybir.dt.int16

CFG = dict(
    N=100000,
    E=1600000,
    IN=256,
    OUT=128,
    NCORES=8,
    GRP=128,          # dst nodes per segment group (= psum partition dim)
    NBANKS=4,         # src banks (bank rows must stay < 32768 for int16 idx)
    BATCH_BLOCKS=40,  # gather batch size in 128-edge blocks
    EPS=1e-5,
    TRACE=False,
)

LAST_RESULTS = None  # set by kernel() for test harness introspection
LAST_NC = None
LAST_RUN_S = None


def _ceil_div(a, b):
    return (a + b - 1) // b


def _wrap16(idx, ncols):
    """int16 idx list -> [128, ncols] tile: idx i at [i%16, i//16], replicated
    8x across the 16-partition groups (one copy per GpSimd Q7 core)."""
    n = idx.shape[0]
    assert n == ncols * 16
    w = np.ascontiguousarray(idx.reshape(ncols, 16).T)
    return np.tile(w, (8, 1))


def _preprocess(cfg, src, dst):
    """Bucket edges by (owner core, src bank, dst group); build per-core
    gather-index / dst-offset arrays and the shared block structure."""
    N, E = cfg["N"], cfg["E"]
    C, NBANKS, GRP = cfg["NCORES"], cfg["NBANKS"], cfg["GRP"]
    NPC = N // C
    NG = _ceil_div(NPC, GRP)
    assert NPC % NBANKS == 0
    QROWS = NPC // NBANKS          # rows per quarter of a core's shard
    BANKROWS = QROWS * C           # rows per bank table (one AllGather output)
    assert BANKROWS < 32768

    src = src.astype(np.int64)
    dst = dst.astype(np.int64)
    deg_out = np.bincount(src, minlength=N).astype(np.float32)
    deg_in = np.bincount(dst, minlength=N).astype(np.float32)

    owner = dst // NPC
    bank = (src % NPC) // QROWS    # quarter index within the source's shard
    grp = (dst % NPC) // GRP
    key = (owner * NBANKS + bank) * NG + grp
    order = np.argsort(key, kind="stable")
    s_src = src[order]
    s_dst = dst[order]
    s_key = key[order]

    counts = np.bincount(key, minlength=C * NBANKS * NG).reshape(C, NBANKS, NG)
    P = counts.max(axis=0)  # [NBANKS, NG]
    P = ((P + 127) // 128) * 128
    P[0] = np.maximum(P[0], 128)  # bank-0 run always exists (initializes agg)

    nidx_tot = int(P.sum())
    nb_tot = nidx_tot // 128
    # stream order: group-chunks outer, banks inner -> the ReLU/BN stage of a
    # chunk's groups can overlap later chunks' gathers
    GC = cfg.get("GCHUNK", 13)
    chunks = [list(range(c, min(c + GC, NG))) for c in range(0, NG, GC)]
    run_seq = [(b, g) for ch in chunks for b in range(NBANKS) for g in ch]
    run_off = np.zeros((NBANKS, NG), np.int64)
    pos = 0
    for b, g in run_seq:
        run_off[b, g] = pos
        pos += P[b, g]

    # boundaries of each (k, b, g) bucket in the sorted edge stream
    bkeys = (np.arange(C)[:, None, None] * NBANKS + np.arange(NBANKS)[None, :, None]) * NG + np.arange(NG)[None, None, :]
    starts = np.searchsorted(s_key, bkeys.ravel()).reshape(C, NBANKS, NG)
    ends = np.searchsorted(s_key, bkeys.ravel(), side="right").reshape(C, NBANKS, NG)

    gidx_cores = []
    dstoff_cores = []
    for k in range(C):
        gidx = np.zeros(nidx_tot, np.int16)
        doff = np.full(nidx_tot, 255.0, np.float32)
        for b in range(NBANKS):
            for g in range(NG):
                s, e = starts[k, b, g], ends[k, b, g]
                cnt = e - s
                if cnt == 0:
                    continue
                p0 = run_off[b, g]
                gidx[p0 : p0 + cnt] = (
                    (s_src[s:e] // NPC) * QROWS + (s_src[s:e] % NPC) % QROWS
                ).astype(np.int16)
                doff[p0 : p0 + cnt] = ((s_dst[s:e] % NPC) - g * GRP).astype(np.float32)
        gidx_cores.append(_wrap16(gidx, nidx_tot // 16))
        # dstoff tile [128, nb_tot]: col t = offsets of block t's 128 edges
        dstoff_cores.append(np.ascontiguousarray(doff.reshape(nb_tot, 128).T))

    # shared static block structure: per block t -> (bank, group, start, stop)
    blocks = []
    for b, g in run_seq:
        nb = P[b, g] // 128
        for j in range(nb):
            blocks.append((b, g, j == 0, j == nb - 1))

    # gather batches: consecutive blocks within one bank, <= BATCH_BLOCKS
    batches = []  # (bank, first_block, n_blocks)
    t = 0
    while t < len(blocks):
        b = blocks[t][0]
        n = 1
        while (
            t + n < len(blocks)
            and blocks[t + n][0] == b
            and n < cfg["BATCH_BLOCKS"]
        ):
            n += 1
        batches.append((b, t, n))
        t += n

    meta = dict(
        NPC=NPC,
        NG=NG,
        BANKROWS=BANKROWS,
        QROWS=QROWS,
        nidx_tot=nidx_tot,
        nb_tot=nb_tot,
        blocks=blocks,
        batches=batches,
        deg_out=deg_out,
        deg_in=deg_in,
    )
    return meta, gidx_cores, dstoff_cores


def _tile_major(vec, NG, GRP, pad_val):
    """[NPC] -> [GRP, NG]: entry (p, m) = vec[m*GRP + p], padded."""
    out = np.full((NG * GRP,), pad_val, vec.dtype)
    out[: vec.shape[0]] = vec
    return np.ascontiguousarray(out.reshape(NG, GRP).T)


def _build_nc(cfg, meta):
    N, IN, OUT, C = cfg["N"], cfg["IN"], cfg["OUT"], cfg["NCORES"]
    GRP, NBANKS = cfg["GRP"], cfg["NBANKS"]
    NPC, NG, BANKROWS = meta["NPC"], meta["NG"], meta["BANKROWS"]
    QROWS = meta["QROWS"]
    nidx_tot, nb_tot = meta["nidx_tot"], meta["nb_tot"]
    blocks, batches = meta["blocks"], meta["batches"]
    XK = _ceil_div(IN, 128)
    assert OUT == 128 and GRP == 128
    last_w = NPC - (NG - 1) * GRP  # valid rows in the last group

    nc = bacc.Bacc(
        "TRN2", target_bir_lowering=False, debug=False, num_devices=C
    )

    # ---- external inputs ----
    xt = [
        nc.dram_tensor(f"xt{j}", [128, NPC], BF16, kind="ExternalInput")
        for j in range(XK)
    ]
    wt = [
        nc.dram_tensor(f"wt{j}", [128, OUT], BF16, kind="ExternalInput")
        for j in range(XK)
    ]
    gidx_d = nc.dram_tensor("gidx", [128, nidx_tot // 16], I16, kind="ExternalInput")
    doff_d = nc.dram_tensor("doff", [128, nb_tot], F32, kind="ExternalInput")
    dego_d = nc.dram_tensor("dego", [128, NG], F32, kind="ExternalInput")
    degi_d = nc.dram_tensor("degi", [128, NG], F32, kind="ExternalInput")
    bt_d = nc.dram_tensor("bt", [128, OUT], F32, kind="ExternalInput")
    iota_d = nc.dram_tensor("iota", [128, GRP], BF16, kind="ExternalInput")
    gm_d = nc.dram_tensor("gm", [1, OUT], F32, kind="ExternalInput")
    bb_d = nc.dram_tensor("bb", [1, OUT], F32, kind="ExternalInput")
    onesc_d = nc.dram_tensor("onesc", [128, 1], F32, kind="ExternalInput")
    onest_d = nc.dram_tensor("onest", [128, 1], F32, kind="ExternalInput")
    onesr_d = nc.dram_tensor("onesr", [1, 128], F32, kind="ExternalInput")

    ypad_d = nc.dram_tensor("ypad", [NG * GRP, OUT], F32, kind="ExternalOutput")

    with tile.TileContext(nc) as tc:
        with (
            tc.tile_pool(name="const", bufs=1) as cpool,
            tc.tile_pool(name="dram", bufs=1, space="DRAM") as dpool,
            tc.tile_pool(name="agg", bufs=1) as apool,
            tc.tile_pool(name="gath", bufs=3) as gpool,
            tc.tile_pool(name="mpool", bufs=6) as mpool,
            tc.tile_pool(name="etmp", bufs=4) as epool,
            tc.tile_pool(name="gtmp", bufs=4) as gpool2,
            tc.tile_pool(name="psum", bufs=3, space="PSUM") as ppool,
            tc.tile_pool(name="pstat", bufs=1, space="PSUM") as pspool,
        ):
            # ---- constants / small tiles ----
            iota_t = cpool.tile([128, GRP], BF16)
            bt_t = cpool.tile([128, OUT], F32)
            dego_t = cpool.tile([128, NG], F32)
            degi_t = cpool.tile([128, NG], F32)
            nsrc_t = cpool.tile([128, NG], F32)
            ndst_t = cpool.tile([128, NG], F32)
            gm_t = cpool.tile([1, OUT], F32)
            bb_t = cpool.tile([1, OUT], F32)
            onesc_t = cpool.tile([128, 1], F32)
            onest_t = cpool.tile([128, 1], F32)
            onesr_t = cpool.tile([1, 128], F32)
            gidx_t = cpool.tile([128, nidx_tot // 16], I16)
            doff_t = cpool.tile([128, nb_tot], F32)

            nc.sync.dma_start(iota_t[:], iota_d[:])
            nc.sync.dma_start(bt_t[:], bt_d[:])
            nc.sync.dma_start(dego_t[:], dego_d[:])
            nc.sync.dma_start(degi_t[:], degi_d[:])
            nc.sync.dma_start(gm_t[:], gm_d[:])
            nc.sync.dma_start(bb_t[:], bb_d[:])
            nc.sync.dma_start(onesc_t[:], onesc_d[:])
            nc.sync.dma_start(onest_t[:], onest_d[:])
            nc.sync.dma_start(onesr_t[:], onesr_d[:])
            nc.sync.dma_start(gidx_t[:], gidx_d[:])
            nc.sync.dma_start(doff_t[:], doff_d[:])

            # norms: rsqrt(max(deg, 1))
            for deg_t, norm_t in ((dego_t, nsrc_t), (degi_t, ndst_t)):
                nc.vector.tensor_scalar(
                    norm_t[:], deg_t[:], 1.0, None, op0=mybir.AluOpType.max
                )
                nc.vector.reciprocal(norm_t[:], norm_t[:])
                nc.scalar.activation(
                    norm_t[:], norm_t[:], mybir.ActivationFunctionType.Sqrt
                )

            # internal DRAM for collectives (quartered for B/C/D pipelining)
            h_my_qs = [
                dpool.tile([QROWS, OUT], BF16, name=f"h_my_{q}")
                for q in range(NBANKS)
            ]
            _aspace = "Local" if cfg.get("NOCC") else "Shared"
            h_all_qs = [
                dpool.tile([BANKROWS, OUT], BF16, addr_space=_aspace, name=f"h_all_{q}")
                for q in range(NBANKS)
            ]
            stats_in = dpool.tile([1, 2 * OUT], F32)
            stats_out = dpool.tile([1, 2 * OUT], F32, addr_space=_aspace)

            agg_t = apool.tile([128, NG, OUT], F32)

            # ---- stage B: h = (x @ W) * norm_src, cast bf16, store to HBM
            with tc.tile_pool(name="xw", bufs=1) as xwp, tc.tile_pool(
                name="hbf", bufs=4
            ) as hbp:
                xts = []
                wts = []
                for j in range(XK):
                    xtile = xwp.tile([128, NPC], BF16, name=f"xt_s{j}")
                    wtile = xwp.tile([128, OUT], BF16, name=f"wt_s{j}")
                    nc.sync.dma_start(xtile[:], xt[j][:])
                    nc.sync.dma_start(wtile[:], wt[j][:])
                    xts.append(xtile)
                    wts.append(wtile)
                for m in range(NG):
                    w = GRP if m < NG - 1 else last_w
                    ps = ppool.tile([128, OUT], F32, tag="hps")
                    for j in range(XK):
                        nc.tensor.matmul(
                            ps[:w, :],
                            xts[j][:, m * GRP : m * GRP + w],
                            wts[j][:, :],
                            start=(j == 0),
                            stop=(j == XK - 1),
                        )
                    hb = hbp.tile([128, OUT], BF16, tag="hb")
                    nc.scalar.activation(
                        hb[:w, :],
                        ps[:w, :],
                        mybir.ActivationFunctionType.Copy,
                        scale=nsrc_t[:w, m : m + 1],
                    )
                    r0 = m * GRP
                    r1 = r0 + w
                    q0 = r0 // QROWS
                    q1 = (r1 - 1) // QROWS
                    for q in range(q0, q1 + 1):
                        a = max(r0, q * QROWS)
                        z = min(r1, (q + 1) * QROWS)
                        nc.sync.dma_start(
                            h_my_qs[q][a - q * QROWS : z - q * QROWS, :],
                            hb[a - r0 : z - r0, :],
                        )

            # ---- stage C: quartered AllGather (pipelines with B and D) ----
            for q in range(NBANKS):
                if cfg.get("NOCC"):
                    rep = (
                        h_my_qs[q][:]
                        .rearrange("(o r) f -> o r f", o=1)
                        .to_broadcast((C, QROWS, OUT))
                    )
                    nc.sync.dma_start(
                        h_all_qs[q][:].rearrange("(o r) f -> o r f", o=C), rep
                    )
                else:
                    nc.gpsimd.collective_compute(
                        "AllGather",
                        mybir.AluOpType.bypass,
                        replica_groups=[list(range(C))],
                        ins=[h_my_qs[q][:]],
                        outs=[h_all_qs[q][:]],
                    )

            # ---- stage D: gather + one-hot matmul segmented sum ----
            stages = cfg.get("STAGES", "BCDEFG")
            if "D" not in stages or cfg.get("DSUB", 3) < 3:
                nc.gpsimd.memset(agg_t[:], 0.0)
            if "D" in stages:
              if True:
                  ps_run = None
                  bmax = max(nb for _, _, nb in batches)
                  for bank, t0, nblk in batches:
                      Gt = gpool.tile([128, bmax, OUT], BF16, tag="G")
                      nc.gpsimd.dma_gather(
                          Gt[:, :nblk, :],
                          h_all_qs[bank][:],
                          gidx_t[:, t0 * 8 : (t0 + nblk) * 8],
                          nblk * 128,
                          nblk * 128,
                          OUT,
                          single_packet=False,
                      )
                      for j in range(nblk):
                          if cfg.get("DSUB", 3) < 2:
                              continue
                          t = t0 + j
                          b, g, is_start, is_stop = blocks[t]
                          Mt = mpool.tile([128, GRP], BF16, tag="M")
                          nc.vector.tensor_scalar(
                              Mt[:],
                              iota_t[:],
                              doff_t[:, t : t + 1],
                              None,
                              op0=mybir.AluOpType.is_equal,
                          )
                          if cfg.get("DSUB", 3) < 3:
                              continue
                          if is_start:
                              ps_run = ppool.tile([128, OUT], F32, tag="aggps")
                          nc.tensor.matmul(
                              ps_run[:],
                              Mt[:],
                              Gt[:, j, :],
                              start=is_start,
                              stop=is_stop,
                          )
                          if is_stop:
                              if b == 0:
                                  nc.scalar.activation(
                                      agg_t[:, g, :],
                                      ps_run[:],
                                      mybir.ActivationFunctionType.Copy,
                                  )
                              else:
                                  nc.vector.tensor_tensor(
                                      agg_t[:, g, :],
                                      agg_t[:, g, :],
                                      ps_run[:],
                                      op=mybir.AluOpType.add,
                                  )

            # ---- stage E: relu(agg*norm_dst + b); BN partial sums ----
            ps_sum = pspool.tile([1, OUT], F32, name="ps_sum")
            ps_sq = pspool.tile([1, OUT], F32, name="ps_sq")
            if "E" in stages:
              if True:
                  for g in range(NG):
                      tmp = epool.tile([128, OUT], F32, tag="etmp")
                      nc.vector.scalar_tensor_tensor(
                          tmp[:],
                          agg_t[:, g, :],
                          ndst_t[:, g : g + 1],
                          bt_t[:],
                          op0=mybir.AluOpType.mult,
                          op1=mybir.AluOpType.add,
                      )
                      nc.scalar.activation(
                          agg_t[:, g, :], tmp[:], mybir.ActivationFunctionType.Relu
                      )
                      ones = onesc_t if g < NG - 1 else onest_t
                      nc.tensor.matmul(
                          ps_sum[:],
                          ones[:],
                          agg_t[:, g, :],
                          start=(g == 0),
                          stop=(g == NG - 1),
                      )
                      sq = epool.tile([128, OUT], F32, tag="esq")
                      nc.scalar.activation(
                          sq[:], agg_t[:, g, :], mybir.ActivationFunctionType.Square
                      )
                      nc.tensor.matmul(
                          ps_sq[:],
                          ones[:],
                          sq[:],
                          start=(g == 0),
                          stop=(g == NG - 1),
                      )

            # ---- stage F: AllReduce BN stats; build affine S/T tiles ----
            S_t = cpool.tile([128, OUT], F32)
            T_t = cpool.tile([128, OUT], F32)
            if "F" not in stages:
                nc.gpsimd.memset(S_t[:], 1.0)
                nc.gpsimd.memset(T_t[:], 0.0)
            if "F" in stages:
              st_sb = cpool.tile([1, 2 * OUT], F32)
              nc.scalar.activation(
                  st_sb[:, 0:OUT], ps_sum[:], mybir.ActivationFunctionType.Copy
              )
              nc.scalar.activation(
                  st_sb[:, OUT : 2 * OUT], ps_sq[:], mybir.ActivationFunctionType.Copy
              )
              nc.sync.dma_start(stats_in[:], st_sb[:])
              if cfg.get("NOCC"):
                  nc.sync.dma_start(stats_out[:], stats_in[:])
              else:
                  nc.gpsimd.collective_compute(
                      "AllReduce",
                      mybir.AluOpType.add,
                      replica_groups=[list(range(C))],
                      ins=[stats_in[:]],
                      outs=[stats_out[:]],
                  )
              st_rb = cpool.tile([1, 2 * OUT], F32)
              nc.sync.dma_start(st_rb[:], stats_out[:])

              mu = cpool.tile([1, OUT], F32)
              ex2 = cpool.tile([1, OUT], F32)
              var = cpool.tile([1, OUT], F32)
              srow = cpool.tile([1, OUT], F32)
              trow = cpool.tile([1, OUT], F32)
              inv_n = 1.0 / float(N)
              nc.scalar.activation(
                  mu[:], st_rb[:, 0:OUT], mybir.ActivationFunctionType.Copy, scale=inv_n
              )
              nc.scalar.activation(
                  ex2[:], st_rb[:, OUT : 2 * OUT], mybir.ActivationFunctionType.Copy, scale=inv_n
              )
              nc.scalar.activation(
                  var[:], mu[:], mybir.ActivationFunctionType.Square
              )
              nc.vector.tensor_sub(var[:], ex2[:], var[:])
              # var <- rsqrt(var + eps) (ACT Rsqrt is banned for accuracy)
              nc.scalar.activation(
                  var[:],
                  var[:],
                  mybir.ActivationFunctionType.Copy,
                  bias=float(cfg["EPS"]),
              )
              nc.vector.reciprocal(var[:], var[:])
              nc.scalar.activation(
                  var[:], var[:], mybir.ActivationFunctionType.Sqrt
              )
              nc.vector.tensor_mul(srow[:], gm_t[:], var[:])
              nc.vector.tensor_mul(trow[:], mu[:], srow[:])
              nc.vector.tensor_sub(trow[:], bb_t[:], trow[:])

              ps_S = ppool.tile([128, OUT], F32, tag="aggps", name="ps_S")
              ps_T = ppool.tile([128, OUT], F32, tag="aggps", name="ps_T")
              nc.tensor.matmul(ps_S[:], onesr_t[:], srow[:], start=True, stop=True)
              nc.tensor.matmul(ps_T[:], onesr_t[:], trow[:], start=True, stop=True)
              nc.scalar.activation(
                  S_t[:], ps_S[:], mybir.ActivationFunctionType.Copy
              )
              nc.scalar.activation(
                  T_t[:], ps_T[:], mybir.ActivationFunctionType.Copy
              )

            # ---- stage G: y = hrelu * S + T, write out ----
            if True:
                for g in range(NG):
                    tmp = gpool2.tile([128, OUT], F32, tag="gtmp")
                    nc.vector.tensor_mul(tmp[:], agg_t[:, g, :], S_t[:])
                    nc.vector.tensor_add(agg_t[:, g, :], tmp[:], T_t[:])
                ypad_view = ypad_d[:].rearrange("(g p) f -> p g f", p=128)
                nc.sync.dma_start(ypad_view, agg_t[:, :, :])

    nc.compile()
    return nc


def kernel(x, src, dst, W, b, gamma, beta):
    global LAST_RESULTS
    cfg = CFG
    N, E, IN, OUT, C = cfg["N"], cfg["E"], cfg["IN"], cfg["OUT"], cfg["NCORES"]
    GRP = cfg["GRP"]
    assert x.shape == (N, IN) and W.shape == (IN, OUT)
    assert src.shape == (E,) and dst.shape == (E,)

    meta, gidx_cores, dstoff_cores = _preprocess(cfg, src, dst)
    NPC, NG = meta["NPC"], meta["NG"]
    XK = _ceil_div(IN, 128)
    last_w = NPC - (NG - 1) * GRP

    nc = _build_nc(cfg, meta)

    xT = np.ascontiguousarray(np.asarray(x, np.float32).T)  # [IN, N]
    Wn = np.asarray(W, np.float32)
    import ml_dtypes

    iota = np.tile(
        np.arange(GRP, dtype=np.float32)[None, :], (128, 1)
    ).astype(ml_dtypes.bfloat16)
    bt = np.tile(np.asarray(b, np.float32)[None, :], (128, 1))
    onesc = np.ones((128, 1), np.float32)
    onest = np.zeros((128, 1), np.float32)
    onest[:last_w] = 1.0
    onesr = np.ones((1, 128), np.float32)
    gm = np.asarray(gamma, np.float32)[None, :]
    bb = np.asarray(beta, np.float32)[None, :]

    in_maps = []
    for k in range(C):
        im = {
            "gidx": gidx_cores[k],
            "doff": dstoff_cores[k],
            "dego": _tile_major(
                meta["deg_out"][k * NPC : (k + 1) * NPC], NG, GRP, np.float32(1.0)
            ),
            "degi": _tile_major(
                meta["deg_in"][k * NPC : (k + 1) * NPC], NG, GRP, np.float32(1.0)
            ),
            "bt": bt,
            "iota": iota,
            "gm": gm,
            "bb": bb,
            "onesc": onesc,
            "onest": onest,
            "onesr": onesr,
        }
        for j in range(XK):
            im[f"xt{j}"] = np.ascontiguousarray(
                xT[j * 128 : (j + 1) * 128, k * NPC : (k + 1) * NPC]
            ).astype(ml_dtypes.bfloat16)
            im[f"wt{j}"] = np.ascontiguousarray(
                Wn[j * 128 : (j + 1) * 128, :]
            ).astype(ml_dtypes.bfloat16)
        in_maps.append(im)

    if cfg.get("SIM"):
        from concourse.bass_interp import MultiCoreSim

        sim = MultiCoreSim(nc, num_cores=C)
        for k, core_sim in sim.cores.items():
            for name, val in in_maps[k].items():
                core_sim.tensor(name)[:] = val
        sim.simulate()
        y = np.empty((N, OUT), np.float32)
        for k in range(C):
            y[k * NPC : (k + 1) * NPC] = sim.cores[k].tensor("ypad")[:NPC]
        return y

    global LAST_NC, LAST_RUN_S
    LAST_NC = nc
    import time as _time

    _t0 = _time.time()
    res = bass_utils.run_bass_kernel_spmd(
        nc,
        in_maps,
        core_ids=list(range(C)),
        trace=cfg.get("TRACE", False),
    )
    LAST_RUN_S = _time.time() - _t0
    LAST_RESULTS = res

    y = np.empty((N, OUT), np.float32)
    for k in range(C):
        y[k * NPC : (k + 1) * NPC] = res.results[k]["ypad"][:NPC]
    return y



# revision 2
# speedup vs baseline: 1.1485x; 1.1485x over previous
"""GCN block (GraphConv + BatchNorm1d + ReLU) on 8 Trainium2 NeuronCores.

v2 strategy — "gather x, apply W after aggregation":

By linearity, agg[dst] = sum_e norm_src[src_e] * x[src_e] @ W
                       = (sum_e norm_src[src_e] * x[src_e]) @ W.
So instead of computing h = x@W on every shard and AllGather-ing the h table
(collectives dominated the v1 timeline),每 core receives the FULL x (bf16,
row-major) in its own HBM and directly dma_gathers raw x rows for its edges.
No AllGather at all, and gathers start at t=0. x rows are 256 bf16 = 512 B,
which also clears the <512 B small-descriptor DMA penalty that h rows
(128 bf16 = 256 B) pay.

Per core k (owns dst nodes [k*NPC, (k+1)*NPC)):
  1. For each 128-edge block (bucketed by (dst-group, src-bank)), gather
     x[src] rows (bf16, batched dma_gather) and segment-sum with one-hot
     matmuls Mw^T @ G accumulated in PSUM. The one-hot mask is scaled by
     w_e = rsqrt(deg_out[src_e]) (tensor_scalar is_equal*mult), folding
     norm_src into the aggregation.
  2. Per finished dst group: agg_x [128,256] PSUM -> bf16 SBUF -> PE
     transpose -> (aggT_j)^T @ W_j accumulated -> out [128, OUT] PSUM.
  3. relu(out * rsqrt(clip(deg_in,1)) + b); local BN sums via ones-matmul;
     AllReduce BN sums; y = (h - mu) * rsqrt(var+eps) * gamma + beta.

Host-side work is limited to integer index bookkeeping (bucketing edges by
(core, src-bank, dst-group), degree counting) and layout transforms (bf16
cast, int16 gather indices). All floating-point math runs on device.

Edges are bucketed by src bank (4 banks of N/4 rows) because dma_gather
indices are int16 (< 32768). Bucket sizes are padded to a structure shared
by all 8 cores so a single SPMD NEFF serves every core; pad slots gather row
0 of the bank and carry a dst offset of 255 -> their one-hot column is all
zeros, so they contribute exactly 0. Banks 0 and 3 are padded to >= 1 block
per group so every group starts in bank 0 and stops in bank 3 (keeps the
BN-stat accumulation chain's start first / stop last in program order).
"""
import math
import os
import sys

sys.path.insert(0, "/opt/trn_rl_repo")

import numpy as np

import concourse.bacc as bacc
import concourse.bass as bass
import concourse.mybir as mybir
import concourse.tile as tile
from concourse import bass_utils

F32 = mybir.dt.float32
BF16 = mybir.dt.bfloat16
I16 = mybir.dt.int16

CFG = dict(
    N=100000,
    E=1600000,
    IN=256,
    OUT=128,
    NCORES=8,
    GRP=128,          # dst nodes per segment group (= psum partition dim)
    NBANKS=4,         # src banks (bank rows must stay < 32768 for int16 idx)
    GCHUNK=4,         # dst groups in flight (1 PSUM bank per open accum chain)
    BATCH_BLOCKS=40,  # gather batch size in 128-edge blocks
    EPS=1e-5,
    TRACE=False,
)

LAST_RESULTS = None  # set by kernel() for test harness introspection
LAST_NC = None
LAST_RUN_S = None


def _ceil_div(a, b):
    return (a + b - 1) // b


def _wrap16(idx, ncols):
    """int16 idx list -> [128, ncols] tile: idx i at [i%16, i//16], replicated
    8x across the 16-partition groups (one copy per GpSimd Q7 core)."""
    n = idx.shape[0]
    assert n == ncols * 16
    w = np.ascontiguousarray(idx.reshape(ncols, 16).T)
    return np.tile(w, (8, 1))


def _preprocess(cfg, src, dst):
    """Bucket edges by (owner core, src bank, dst group); build per-core
    gather-index / dst-offset / src-degree arrays and the shared block
    structure."""
    N, E = cfg["N"], cfg["E"]
    C, NBANKS, GRP = cfg["NCORES"], cfg["NBANKS"], cfg["GRP"]
    NPC = N // C
    NG = _ceil_div(NPC, GRP)
    assert N % NBANKS == 0
    BROWS = N // NBANKS            # rows per x bank (gather source table)
    assert BROWS < 32768

    src = src.astype(np.int64)
    dst = dst.astype(np.int64)
    deg_out = np.bincount(src, minlength=N).astype(np.float32)
    deg_in = np.bincount(dst, minlength=N).astype(np.float32)

    owner = dst // NPC
    bank = src // BROWS
    grp = (dst % NPC) // GRP
    key = (owner * NBANKS + bank) * NG + grp
    order = np.argsort(key, kind="stable")
    s_src = src[order]
    s_dst = dst[order]
    s_key = key[order]

    counts = np.bincount(key, minlength=C * NBANKS * NG).reshape(C, NBANKS, NG)
    P = counts.max(axis=0)  # [NBANKS, NG]
    P = ((P + 127) // 128) * 128
    P[0] = np.maximum(P[0], 128)   # every group starts in bank 0
    P[NBANKS - 1] = np.maximum(P[NBANKS - 1], 128)  # ... and stops in bank 3

    nidx_tot = int(P.sum())
    nb_tot = nidx_tot // 128
    # stream order: group-chunks outer, banks inner; a group's PSUM slot is
    # live across all banks of its chunk (accumulated with start/stop)
    GC = cfg["GCHUNK"]
    chunks = [list(range(c, min(c + GC, NG))) for c in range(0, NG, GC)]
    run_seq = [(b, g) for ch in chunks for b in range(NBANKS) for g in ch]
    run_off = np.zeros((NBANKS, NG), np.int64)
    pos = 0
    for b, g in run_seq:
        run_off[b, g] = pos
        pos += P[b, g]

    # boundaries of each (k, b, g) bucket in the sorted edge stream
    bkeys = (np.arange(C)[:, None, None] * NBANKS + np.arange(NBANKS)[None, :, None]) * NG + np.arange(NG)[None, None, :]
    starts = np.searchsorted(s_key, bkeys.ravel()).reshape(C, NBANKS, NG)
    ends = np.searchsorted(s_key, bkeys.ravel(), side="right").reshape(C, NBANKS, NG)

    gidx_cores = []
    dstoff_cores = []
    wdeg_cores = []
    for k in range(C):
        gidx = np.zeros(nidx_tot, np.int16)
        doff = np.full(nidx_tot, 255.0, np.float32)
        wdeg = np.ones(nidx_tot, np.float32)
        for b in range(NBANKS):
            for g in range(NG):
                s, e = starts[k, b, g], ends[k, b, g]
                cnt = e - s
                if cnt == 0:
                    continue
                p0 = run_off[b, g]
                gidx[p0 : p0 + cnt] = (s_src[s:e] % BROWS).astype(np.int16)
                doff[p0 : p0 + cnt] = ((s_dst[s:e] % NPC) - g * GRP).astype(np.float32)
                wdeg[p0 : p0 + cnt] = deg_out[s_src[s:e]]
        gidx_cores.append(_wrap16(gidx, nidx_tot // 16))
        # doff/wdeg tiles [128, nb_tot]: col t = values of block t's 128 edges
        dstoff_cores.append(np.ascontiguousarray(doff.reshape(nb_tot, 128).T))
        wdeg_cores.append(np.ascontiguousarray(wdeg.reshape(nb_tot, 128).T))

    # shared static block structure: per block t -> (bank, group, start, stop)
    # start: first block of the group overall (bank 0); stop: last (bank 3)
    blocks = []
    for b, g in run_seq:
        nb = P[b, g] // 128
        for j in range(nb):
            blocks.append(
                (b, g, b == 0 and j == 0, b == NBANKS - 1 and j == nb - 1)
            )

    # gather batches: consecutive blocks within one bank, <= BATCH_BLOCKS
    batches = []  # (bank, first_block, n_blocks)
    t = 0
    while t < len(blocks):
        b = blocks[t][0]
        n = 1
        while (
            t + n < len(blocks)
            and blocks[t + n][0] == b
            and n < cfg["BATCH_BLOCKS"]
        ):
            n += 1
        batches.append((b, t, n))
        t += n

    meta = dict(
        NPC=NPC,
        NG=NG,
        BROWS=BROWS,
        nidx_tot=nidx_tot,
        nb_tot=nb_tot,
        blocks=blocks,
        batches=batches,
        deg_in=deg_in,
    )
    return meta, gidx_cores, dstoff_cores, wdeg_cores


def _tile_major(vec, NG, GRP, pad_val):
    """[NPC] -> [GRP, NG]: entry (p, m) = vec[m*GRP + p], padded."""
    out = np.full((NG * GRP,), pad_val, vec.dtype)
    out[: vec.shape[0]] = vec
    return np.ascontiguousarray(out.reshape(NG, GRP).T)


def _build_nc(cfg, meta):
    N, IN, OUT, C = cfg["N"], cfg["IN"], cfg["OUT"], cfg["NCORES"]
    GRP, NBANKS, GC = cfg["GRP"], cfg["NBANKS"], cfg["GCHUNK"]
    NPC, NG, BROWS = meta["NPC"], meta["NG"], meta["BROWS"]
    nidx_tot, nb_tot = meta["nidx_tot"], meta["nb_tot"]
    blocks, batches = meta["blocks"], meta["batches"]
    XK = _ceil_div(IN, 128)
    assert OUT == 128 and GRP == 128 and XK == 2
    last_w = NPC - (NG - 1) * GRP  # valid rows in the last group

    nc = bacc.Bacc(
        "TRN2", target_bir_lowering=False, debug=False, num_devices=C
    )

    # ---- external inputs ----
    xb = [
        nc.dram_tensor(f"xb{q}", [BROWS, IN], BF16, kind="ExternalInput")
        for q in range(NBANKS)
    ]
    wt = [
        nc.dram_tensor(f"wt{j}", [128, OUT], BF16, kind="ExternalInput")
        for j in range(XK)
    ]
    gidx_d = nc.dram_tensor("gidx", [128, nidx_tot // 16], I16, kind="ExternalInput")
    doff_d = nc.dram_tensor("doff", [128, nb_tot], F32, kind="ExternalInput")
    wdeg_d = nc.dram_tensor("wdeg", [128, nb_tot], F32, kind="ExternalInput")
    degi_d = nc.dram_tensor("degi", [128, NG], F32, kind="ExternalInput")
    bt_d = nc.dram_tensor("bt", [128, OUT], F32, kind="ExternalInput")
    iota_d = nc.dram_tensor("iota", [128, GRP], BF16, kind="ExternalInput")
    ident_d = nc.dram_tensor("ident", [128, 128], BF16, kind="ExternalInput")
    gm_d = nc.dram_tensor("gm", [1, OUT], F32, kind="ExternalInput")
    bb_d = nc.dram_tensor("bb", [1, OUT], F32, kind="ExternalInput")
    onesc_d = nc.dram_tensor("onesc", [128, 1], F32, kind="ExternalInput")
    onest_d = nc.dram_tensor("onest", [128, 1], F32, kind="ExternalInput")
    onesr_d = nc.dram_tensor("onesr", [1, 128], F32, kind="ExternalInput")

    ypad_d = nc.dram_tensor("ypad", [NG * GRP, OUT], F32, kind="ExternalOutput")

    with tile.TileContext(nc) as tc:
        with (
            tc.tile_pool(name="const", bufs=1) as cpool,
            tc.tile_pool(name="dram", bufs=1, space="DRAM") as dpool,
            tc.tile_pool(name="hrelu", bufs=1) as hpool,
            tc.tile_pool(name="gath", bufs=3) as gpool,
            tc.tile_pool(name="mpool", bufs=6) as mpool,
            tc.tile_pool(name="capool", bufs=4) as capool,
            tc.tile_pool(name="etmp", bufs=4) as epool,
            tc.tile_pool(name="gtmp", bufs=4) as gpool2,
            tc.tile_pool(name="psagg", bufs=1, space="PSUM") as pagg,
            tc.tile_pool(name="psaux", bufs=1, space="PSUM") as paux,
            tc.tile_pool(name="pstat", bufs=1, space="PSUM") as pspool,
        ):
            # ---- constants / small tiles ----
            iota_t = cpool.tile([128, GRP], BF16)
            ident_t = cpool.tile([128, 128], BF16)
            bt_t = cpool.tile([128, OUT], F32)
            degi_t = cpool.tile([128, NG], F32)
            ndst_t = cpool.tile([128, NG], F32)
            gm_t = cpool.tile([1, OUT], F32)
            bb_t = cpool.tile([1, OUT], F32)
            onesc_t = cpool.tile([128, 1], F32)
            onest_t = cpool.tile([128, 1], F32)
            onesr_t = cpool.tile([1, 128], F32)
            gidx_t = cpool.tile([128, nidx_tot // 16], I16)
            doff_t = cpool.tile([128, nb_tot], F32)
            wsrc_t = cpool.tile([128, nb_tot], F32)
            wts = [cpool.tile([128, OUT], BF16, name=f"wt_s{j}") for j in range(XK)]

            nc.sync.dma_start(gidx_t[:], gidx_d[:])
            nc.sync.dma_start(doff_t[:], doff_d[:])
            nc.sync.dma_start(wsrc_t[:], wdeg_d[:])
            nc.sync.dma_start(iota_t[:], iota_d[:])
            nc.sync.dma_start(ident_t[:], ident_d[:])
            nc.sync.dma_start(bt_t[:], bt_d[:])
            nc.sync.dma_start(degi_t[:], degi_d[:])
            nc.sync.dma_start(gm_t[:], gm_d[:])
            nc.sync.dma_start(bb_t[:], bb_d[:])
            nc.sync.dma_start(onesc_t[:], onesc_d[:])
            nc.sync.dma_start(onest_t[:], onest_d[:])
            nc.sync.dma_start(onesr_t[:], onesr_d[:])
            for j in range(XK):
                nc.sync.dma_start(wts[j][:], wt[j][:])

            # per-edge src norm: w = rsqrt(deg_out[src]) (pad slots carry 1.0)
            nc.vector.reciprocal(wsrc_t[:], wsrc_t[:])
            nc.scalar.activation(
                wsrc_t[:], wsrc_t[:], mybir.ActivationFunctionType.Sqrt
            )
            # dst norm: rsqrt(max(deg_in, 1)) tile-major [GRP, NG]
            nc.vector.tensor_scalar(
                ndst_t[:], degi_t[:], 1.0, None, op0=mybir.AluOpType.max
            )
            nc.vector.reciprocal(ndst_t[:], ndst_t[:])
            nc.scalar.activation(
                ndst_t[:], ndst_t[:], mybir.ActivationFunctionType.Sqrt
            )

            stats_in = dpool.tile([1, 2 * OUT], F32)
            _aspace = "Local" if cfg.get("NOCC") else "Shared"
            stats_out = dpool.tile([1, 2 * OUT], F32, addr_space=_aspace)

            hrelu_t = hpool.tile([128, NG, OUT], F32)

            # ---- PSUM layout (8 banks x 2KB); accumulation-group zeroing is
            # bank-granular, so every concurrently-open chain gets its own
            # bank: 4x agg (GCHUNK groups in flight) + 1x transpose + 2x out
            # (alternating, WAR-tracked) + 1x BN stats (sum+sq as one chain).
            assert GC == 4
            ps_agg = [
                pagg.tile([128, 2 * OUT], F32, name=f"ps_agg{i}") for i in range(GC)
            ]
            ps_tr = paux.tile([128, 2, OUT], BF16, name="ps_tr")
            ps_out = [
                paux.tile([128, OUT], F32, name=f"ps_out{i}") for i in range(2)
            ]
            ps_stat = pspool.tile([1, 2, OUT], F32, name="ps_stat")

            # ---- stage D: gather x rows + one-hot matmul segmented sum ----
            bmax = max(nb for _, _, nb in batches)
            for bank, t0, nblk in batches:
                Gt = gpool.tile([128, bmax, IN], BF16, tag="G")
                nc.gpsimd.dma_gather(
                    Gt[:, :nblk, :],
                    xb[bank][:],
                    gidx_t[:, t0 * 8 : (t0 + nblk) * 8],
                    nblk * 128,
                    nblk * 128,
                    IN,
                    single_packet=False,
                )
                for j in range(nblk):
                    t = t0 + j
                    b, g, is_start, is_stop = blocks[t]
                    gi = g % GC
                    Mt = mpool.tile([128, GRP], BF16, tag="M")
                    nc.vector.tensor_scalar(
                        Mt[:],
                        iota_t[:],
                        doff_t[:, t : t + 1],
                        wsrc_t[:, t : t + 1],
                        op0=mybir.AluOpType.is_equal,
                        op1=mybir.AluOpType.mult,
                    )
                    nc.tensor.matmul(
                        ps_agg[gi][:],
                        Mt[:],
                        Gt[:, j, :],
                        start=is_start,
                        stop=is_stop,
                    )
                    if not is_stop:
                        continue
                    # ---- group g complete: apply W, then relu/BN partials
                    cagg = capool.tile([128, 2 * OUT], BF16, tag="cagg")
                    nc.scalar.activation(
                        cagg[:], ps_agg[gi][:], mybir.ActivationFunctionType.Copy
                    )
                    for h in range(2):
                        nc.tensor.matmul(
                            ps_tr[:, h, :],
                            cagg[:, h * OUT : (h + 1) * OUT],
                            ident_t[:],
                            is_transpose=True,
                            start=(h == 0),
                            stop=(h == 1),
                        )
                    # one whole-tile copy so the next group's transpose chain
                    # (which pending-zeroes the full bank) WAR-waits on it
                    tagg = capool.tile([128, 2, OUT], BF16, tag="tagg")
                    nc.scalar.activation(
                        tagg[:, :, :], ps_tr[:, :, :], mybir.ActivationFunctionType.Copy
                    )
                    po = ps_out[g % 2]
                    for jj in range(XK):
                        nc.tensor.matmul(
                            po[:],
                            tagg[:, jj, :],
                            wts[jj][:],
                            start=(jj == 0),
                            stop=(jj == XK - 1),
                        )
                    # stage E: relu(out * ndst + b), BN partial sums
                    tmp = epool.tile([128, OUT], F32, tag="etmp")
                    nc.vector.scalar_tensor_tensor(
                        tmp[:],
                        po[:],
                        ndst_t[:, g : g + 1],
                        bt_t[:],
                        op0=mybir.AluOpType.mult,
                        op1=mybir.AluOpType.add,
                    )
                    nc.scalar.activation(
                        hrelu_t[:, g, :], tmp[:], mybir.ActivationFunctionType.Relu
                    )
                    ones = onesc_t if g < NG - 1 else onest_t
                    nc.tensor.matmul(
                        ps_stat[:, 0, :],
                        ones[:],
                        hrelu_t[:, g, :],
                        start=(g == 0),
                        stop=False,
                    )
                    sq = epool.tile([128, OUT], F32, tag="esq")
                    nc.scalar.activation(
                        sq[:], hrelu_t[:, g, :], mybir.ActivationFunctionType.Square
                    )
                    nc.tensor.matmul(
                        ps_stat[:, 1, :],
                        ones[:],
                        sq[:],
                        start=False,
                        stop=(g == NG - 1),
                    )

            # ---- stage F: AllReduce BN stats; build affine S/T tiles ----
            S_t = cpool.tile([128, OUT], F32)
            T_t = cpool.tile([128, OUT], F32)
            st_sb = cpool.tile([1, 2 * OUT], F32)
            nc.scalar.activation(
                st_sb[:], ps_stat[:].rearrange("p a f -> p (a f)"),
                mybir.ActivationFunctionType.Copy,
            )
            nc.sync.dma_start(stats_in[:], st_sb[:])
            if cfg.get("NOCC"):
                nc.sync.dma_start(stats_out[:], stats_in[:])
            else:
                nc.gpsimd.collective_compute(
                    "AllReduce",
                    mybir.AluOpType.add,
                    replica_groups=[list(range(C))],
                    ins=[stats_in[:]],
                    outs=[stats_out[:]],
                )
            st_rb = cpool.tile([1, 2 * OUT], F32)
            nc.sync.dma_start(st_rb[:], stats_out[:])

            mu = cpool.tile([1, OUT], F32)
            ex2 = cpool.tile([1, OUT], F32)
            var = cpool.tile([1, OUT], F32)
            srow = cpool.tile([1, OUT], F32)
            trow = cpool.tile([1, OUT], F32)
            inv_n = 1.0 / float(N)
            nc.scalar.activation(
                mu[:], st_rb[:, 0:OUT], mybir.ActivationFunctionType.Copy, scale=inv_n
            )
            nc.scalar.activation(
                ex2[:], st_rb[:, OUT : 2 * OUT], mybir.ActivationFunctionType.Copy, scale=inv_n
            )
            nc.scalar.activation(
                var[:], mu[:], mybir.ActivationFunctionType.Square
            )
            nc.vector.tensor_sub(var[:], ex2[:], var[:])
            # var <- rsqrt(var + eps) (ACT Rsqrt is banned for accuracy)
            nc.scalar.activation(
                var[:],
                var[:],
                mybir.ActivationFunctionType.Copy,
                bias=float(cfg["EPS"]),
            )
            nc.vector.reciprocal(var[:], var[:])
            nc.scalar.activation(
                var[:], var[:], mybir.ActivationFunctionType.Sqrt
            )
            nc.vector.tensor_mul(srow[:], gm_t[:], var[:])
            nc.vector.tensor_mul(trow[:], mu[:], srow[:])
            nc.vector.tensor_sub(trow[:], bb_t[:], trow[:])

            # reuse the (now idle) out banks for the S/T broadcast matmuls
            nc.tensor.matmul(ps_out[0][:], onesr_t[:], srow[:], start=True, stop=True)
            nc.scalar.activation(
                S_t[:], ps_out[0][:], mybir.ActivationFunctionType.Copy
            )
            nc.tensor.matmul(ps_out[1][:], onesr_t[:], trow[:], start=True, stop=True)
            nc.scalar.activation(
                T_t[:], ps_out[1][:], mybir.ActivationFunctionType.Copy
            )

            # ---- stage G: y = hrelu * S + T, write out ----
            for g in range(NG):
                tmp = gpool2.tile([128, OUT], F32, tag="gtmp")
                nc.vector.tensor_mul(tmp[:], hrelu_t[:, g, :], S_t[:])
                nc.vector.tensor_add(hrelu_t[:, g, :], tmp[:], T_t[:])
            ypad_view = ypad_d[:].rearrange("(g p) f -> p g f", p=128)
            nc.sync.dma_start(ypad_view, hrelu_t[:, :, :])

    nc.compile()
    return nc


def kernel(x, src, dst, W, b, gamma, beta):
    global LAST_RESULTS
    cfg = CFG
    N, E, IN, OUT, C = cfg["N"], cfg["E"], cfg["IN"], cfg["OUT"], cfg["NCORES"]
    GRP = cfg["GRP"]
    assert x.shape == (N, IN) and W.shape == (IN, OUT)
    assert src.shape == (E,) and dst.shape == (E,)

    meta, gidx_cores, dstoff_cores, wdeg_cores = _preprocess(cfg, src, dst)
    NPC, NG, BROWS = meta["NPC"], meta["NG"], meta["BROWS"]
    XK = _ceil_div(IN, 128)
    last_w = NPC - (NG - 1) * GRP

    nc = _build_nc(cfg, meta)

    import ml_dtypes

    x_bf = np.asarray(x, np.float32).astype(ml_dtypes.bfloat16)
    Wn = np.asarray(W, np.float32)

    iota = np.tile(
        np.arange(GRP, dtype=np.float32)[None, :], (128, 1)
    ).astype(ml_dtypes.bfloat16)
    ident = np.eye(128, dtype=np.float32).astype(ml_dtypes.bfloat16)
    bt = np.tile(np.asarray(b, np.float32)[None, :], (128, 1))
    onesc = np.ones((128, 1), np.float32)
    onest = np.zeros((128, 1), np.float32)
    onest[:last_w] = 1.0
    onesr = np.ones((1, 128), np.float32)
    gm = np.asarray(gamma, np.float32)[None, :]
    bb = np.asarray(beta, np.float32)[None, :]
    xbanks = [
        np.ascontiguousarray(x_bf[q * BROWS : (q + 1) * BROWS])
        for q in range(cfg["NBANKS"])
    ]
    wtiles = [
        np.ascontiguousarray(Wn[j * 128 : (j + 1) * 128, :]).astype(
            ml_dtypes.bfloat16
        )
        for j in range(XK)
    ]

    in_maps = []
    for k in range(C):
        im = {
            "gidx": gidx_cores[k],
            "doff": dstoff_cores[k],
            "wdeg": wdeg_cores[k],
            "degi": _tile_major(
                meta["deg_in"][k * NPC : (k + 1) * NPC], NG, GRP, np.float32(1.0)
            ),
            "bt": bt,
            "iota": iota,
            "ident": ident,
            "gm": gm,
            "bb": bb,
            "onesc": onesc,
            "onest": onest,
            "onesr": onesr,
        }
        for q in range(cfg["NBANKS"]):
            im[f"xb{q}"] = xbanks[q]
        for j in range(XK):
            im[f"wt{j}"] = wtiles[j]
        in_maps.append(im)

    if cfg.get("SIM"):
        from concourse.bass_interp import MultiCoreSim

        sim = MultiCoreSim(nc, num_cores=C)
        for k, core_sim in sim.cores.items():
            for name, val in in_maps[k].items():
                core_sim.tensor(name)[:] = val
        sim.simulate()
        y = np.empty((N, OUT), np.float32)
        for k in range(C):
            y[k * NPC : (k + 1) * NPC] = sim.cores[k].tensor("ypad")[:NPC]
        return y

    global LAST_NC, LAST_RUN_S
    LAST_NC = nc
    import time as _time

    _t0 = _time.time()
    res = bass_utils.run_bass_kernel_spmd(
        nc,
        in_maps,
        core_ids=list(range(C)),
        trace=cfg.get("TRACE", False),
    )
    LAST_RUN_S = _time.time() - _t0
    LAST_RESULTS = res

    y = np.empty((N, OUT), np.float32)
    for k in range(C):
        y[k * NPC : (k + 1) * NPC] = res.results[k]["ypad"][:NPC]
    return y


# revision 3
# speedup vs baseline: 1.1901x; 1.0362x over previous
"""GCN block (GraphConv + BatchNorm1d + ReLU) on 8 Trainium2 NeuronCores.

v2 strategy — "gather x, apply W after aggregation":

By linearity, agg[dst] = sum_e norm_src[src_e] * x[src_e] @ W
                       = (sum_e norm_src[src_e] * x[src_e]) @ W.
So instead of computing h = x@W on every shard and AllGather-ing the h table
(collectives dominated the v1 timeline),每 core receives the FULL x (bf16,
row-major) in its own HBM and directly dma_gathers raw x rows for its edges.
No AllGather at all, and gathers start at t=0. x rows are 256 bf16 = 512 B,
which also clears the <512 B small-descriptor DMA penalty that h rows
(128 bf16 = 256 B) pay.

Per core k (owns dst nodes [k*NPC, (k+1)*NPC)):
  1. For each 128-edge block (bucketed by (dst-group, src-bank)), gather
     x[src] rows (bf16, batched dma_gather) and segment-sum with one-hot
     matmuls Mw^T @ G accumulated in PSUM. The one-hot mask is scaled by
     w_e = rsqrt(deg_out[src_e]) (tensor_scalar is_equal*mult), folding
     norm_src into the aggregation.
  2. Per finished dst group: agg_x [128,256] PSUM -> bf16 SBUF -> PE
     transpose -> (aggT_j)^T @ W_j accumulated -> out [128, OUT] PSUM.
  3. relu(out * rsqrt(clip(deg_in,1)) + b); local BN sums via ones-matmul;
     AllReduce BN sums; y = (h - mu) * rsqrt(var+eps) * gamma + beta.

Host-side work is limited to integer index bookkeeping (bucketing edges by
(core, src-bank, dst-group), degree counting) and layout transforms (bf16
cast, int16 gather indices). All floating-point math runs on device.

Edges are bucketed by src bank (4 banks of N/4 rows) because dma_gather
indices are int16 (< 32768). Bucket sizes are padded to a structure shared
by all 8 cores so a single SPMD NEFF serves every core; pad slots gather row
0 of the bank and carry a dst offset of 255 -> their one-hot column is all
zeros, so they contribute exactly 0. Banks 0 and 3 are padded to >= 1 block
per group so every group starts in bank 0 and stops in bank 3 (keeps the
BN-stat accumulation chain's start first / stop last in program order).
"""
import math
import os
import sys

sys.path.insert(0, "/opt/trn_rl_repo")

import numpy as np

import concourse.bacc as bacc
import concourse.bass as bass
import concourse.mybir as mybir
import concourse.tile as tile
from concourse import bass_utils

F32 = mybir.dt.float32
BF16 = mybir.dt.bfloat16
I16 = mybir.dt.int16

CFG = dict(
    N=100000,
    E=1600000,
    IN=256,
    OUT=128,
    NCORES=8,
    GRP=128,          # dst nodes per segment group (= psum partition dim)
    NBANKS=4,         # src banks (bank rows must stay < 32768 for int16 idx)
    GCHUNK=4,         # dst groups in flight (1 PSUM bank per open accum chain)
    BATCH_BLOCKS=40,  # gather batch size in 128-edge blocks
    EPS=1e-5,
    TRACE=False,
)

LAST_RESULTS = None  # set by kernel() for test harness introspection
LAST_NC = None
LAST_RUN_S = None


def _ceil_div(a, b):
    return (a + b - 1) // b


def _wrap16(idx, ncols):
    """int16 idx list -> [128, ncols] tile: idx i at [i%16, i//16], replicated
    8x across the 16-partition groups (one copy per GpSimd Q7 core)."""
    n = idx.shape[0]
    assert n == ncols * 16
    w = np.ascontiguousarray(idx.reshape(ncols, 16).T)
    return np.tile(w, (8, 1))


def _preprocess(cfg, src, dst):
    """Bucket edges by (owner core, src bank, dst group); build per-core
    gather-index / dst-offset / src-degree arrays and the shared block
    structure."""
    N, E = cfg["N"], cfg["E"]
    C, NBANKS, GRP = cfg["NCORES"], cfg["NBANKS"], cfg["GRP"]
    NPC = N // C
    NG = _ceil_div(NPC, GRP)
    assert N % NBANKS == 0
    BROWS = N // NBANKS            # rows per x bank (gather source table)
    assert BROWS < 32768

    src = src.astype(np.int64)
    dst = dst.astype(np.int64)
    deg_out = np.bincount(src, minlength=N).astype(np.float32)
    deg_in = np.bincount(dst, minlength=N).astype(np.float32)

    owner = dst // NPC
    bank = src // BROWS
    grp = (dst % NPC) // GRP
    key = (owner * NBANKS + bank) * NG + grp
    order = np.argsort(key, kind="stable")
    s_src = src[order]
    s_dst = dst[order]
    s_key = key[order]

    counts = np.bincount(key, minlength=C * NBANKS * NG).reshape(C, NBANKS, NG)
    P = counts.max(axis=0)  # [NBANKS, NG]
    P = ((P + 127) // 128) * 128
    P[0] = np.maximum(P[0], 128)   # every group starts in bank 0
    P[NBANKS - 1] = np.maximum(P[NBANKS - 1], 128)  # ... and stops in bank 3

    nidx_tot = int(P.sum())
    nb_tot = nidx_tot // 128
    # stream order: group-chunks outer, banks inner; a group's PSUM slot is
    # live across all banks of its chunk (accumulated with start/stop)
    GC = cfg["GCHUNK"]
    chunks = [list(range(c, min(c + GC, NG))) for c in range(0, NG, GC)]
    run_seq = [(b, g) for ch in chunks for b in range(NBANKS) for g in ch]
    run_off = np.zeros((NBANKS, NG), np.int64)
    pos = 0
    for b, g in run_seq:
        run_off[b, g] = pos
        pos += P[b, g]

    # boundaries of each (k, b, g) bucket in the sorted edge stream
    bkeys = (np.arange(C)[:, None, None] * NBANKS + np.arange(NBANKS)[None, :, None]) * NG + np.arange(NG)[None, None, :]
    starts = np.searchsorted(s_key, bkeys.ravel()).reshape(C, NBANKS, NG)
    ends = np.searchsorted(s_key, bkeys.ravel(), side="right").reshape(C, NBANKS, NG)

    gidx_cores = []
    dstoff_cores = []
    wdeg_cores = []
    for k in range(C):
        gidx = np.zeros(nidx_tot, np.int16)
        doff = np.full(nidx_tot, 255.0, np.float32)
        wdeg = np.ones(nidx_tot, np.float32)
        for b in range(NBANKS):
            for g in range(NG):
                s, e = starts[k, b, g], ends[k, b, g]
                cnt = e - s
                if cnt == 0:
                    continue
                p0 = run_off[b, g]
                gidx[p0 : p0 + cnt] = (s_src[s:e] % BROWS).astype(np.int16)
                doff[p0 : p0 + cnt] = ((s_dst[s:e] % NPC) - g * GRP).astype(np.float32)
                wdeg[p0 : p0 + cnt] = deg_out[s_src[s:e]]
        gidx_cores.append(_wrap16(gidx, nidx_tot // 16))
        # doff/wdeg tiles [128, nb_tot]: col t = values of block t's 128 edges
        dstoff_cores.append(np.ascontiguousarray(doff.reshape(nb_tot, 128).T))
        wdeg_cores.append(np.ascontiguousarray(wdeg.reshape(nb_tot, 128).T))

    # shared static block structure: per block t -> (bank, group, start, stop)
    # start: first block of the group overall (bank 0); stop: last (bank 3)
    blocks = []
    for b, g in run_seq:
        nb = P[b, g] // 128
        for j in range(nb):
            blocks.append(
                (b, g, b == 0 and j == 0, b == NBANKS - 1 and j == nb - 1)
            )

    # gather batches: consecutive blocks within one bank, <= BATCH_BLOCKS
    batches = []  # (bank, first_block, n_blocks)
    t = 0
    while t < len(blocks):
        b = blocks[t][0]
        n = 1
        while (
            t + n < len(blocks)
            and blocks[t + n][0] == b
            and n < cfg["BATCH_BLOCKS"]
        ):
            n += 1
        batches.append((b, t, n))
        t += n

    meta = dict(
        NPC=NPC,
        NG=NG,
        BROWS=BROWS,
        nidx_tot=nidx_tot,
        nb_tot=nb_tot,
        blocks=blocks,
        batches=batches,
        deg_in=deg_in,
    )
    return meta, gidx_cores, dstoff_cores, wdeg_cores


def _tile_major(vec, NG, GRP, pad_val):
    """[NPC] -> [GRP, NG]: entry (p, m) = vec[m*GRP + p], padded."""
    out = np.full((NG * GRP,), pad_val, vec.dtype)
    out[: vec.shape[0]] = vec
    return np.ascontiguousarray(out.reshape(NG, GRP).T)


def _build_nc(cfg, meta):
    N, IN, OUT, C = cfg["N"], cfg["IN"], cfg["OUT"], cfg["NCORES"]
    GRP, NBANKS, GC = cfg["GRP"], cfg["NBANKS"], cfg["GCHUNK"]
    NPC, NG, BROWS = meta["NPC"], meta["NG"], meta["BROWS"]
    nidx_tot, nb_tot = meta["nidx_tot"], meta["nb_tot"]
    blocks, batches = meta["blocks"], meta["batches"]
    XK = _ceil_div(IN, 128)
    assert OUT == 128 and GRP == 128 and XK == 2
    last_w = NPC - (NG - 1) * GRP  # valid rows in the last group

    nc = bacc.Bacc(
        "TRN2", target_bir_lowering=False, debug=False, num_devices=C
    )

    # ---- external inputs ----
    xb = [
        nc.dram_tensor(f"xb{q}", [BROWS, IN], BF16, kind="ExternalInput")
        for q in range(NBANKS)
    ]
    wt = [
        nc.dram_tensor(f"wt{j}", [128, OUT], BF16, kind="ExternalInput")
        for j in range(XK)
    ]
    gidx_d = nc.dram_tensor("gidx", [128, nidx_tot // 16], I16, kind="ExternalInput")
    doff_d = nc.dram_tensor("doff", [128, nb_tot], F32, kind="ExternalInput")
    wdeg_d = nc.dram_tensor("wdeg", [128, nb_tot], F32, kind="ExternalInput")
    degi_d = nc.dram_tensor("degi", [128, NG], F32, kind="ExternalInput")
    bt_d = nc.dram_tensor("bt", [128, OUT], F32, kind="ExternalInput")
    iota_d = nc.dram_tensor("iota", [128, GRP], BF16, kind="ExternalInput")
    ident_d = nc.dram_tensor("ident", [128, 128], BF16, kind="ExternalInput")
    gm_d = nc.dram_tensor("gm", [1, OUT], F32, kind="ExternalInput")
    bb_d = nc.dram_tensor("bb", [1, OUT], F32, kind="ExternalInput")
    onesc_d = nc.dram_tensor("onesc", [128, 1], F32, kind="ExternalInput")
    ones8_d = nc.dram_tensor("ones8", [8, 1], F32, kind="ExternalInput")
    onest_d = nc.dram_tensor("onest", [128, 1], F32, kind="ExternalInput")
    onesr_d = nc.dram_tensor("onesr", [1, 128], F32, kind="ExternalInput")

    ypad_d = nc.dram_tensor("ypad", [NG * GRP, OUT], F32, kind="ExternalOutput")

    with tile.TileContext(nc) as tc:
        with (
            tc.tile_pool(name="const", bufs=1) as cpool,
            tc.tile_pool(name="dram", bufs=1, space="DRAM") as dpool,
            tc.tile_pool(name="hrelu", bufs=1) as hpool,
            tc.tile_pool(name="gath", bufs=4) as gpool,
            tc.tile_pool(name="mpool", bufs=6) as mpool,
            tc.tile_pool(name="capool", bufs=4) as capool,
            tc.tile_pool(name="etmp", bufs=4) as epool,
            tc.tile_pool(name="gtmp", bufs=4) as gpool2,
            tc.tile_pool(name="psagg", bufs=1, space="PSUM") as pagg,
            tc.tile_pool(name="psaux", bufs=1, space="PSUM") as paux,
            tc.tile_pool(name="pstat", bufs=1, space="PSUM") as pspool,
        ):
            # ---- constants / small tiles ----
            iota_t = cpool.tile([128, GRP], BF16)
            ident_t = cpool.tile([128, 128], BF16)
            bt_t = cpool.tile([128, OUT], F32)
            degi_t = cpool.tile([128, NG], F32)
            ndst_t = cpool.tile([128, NG], F32)
            gm_t = cpool.tile([1, OUT], F32)
            bb_t = cpool.tile([1, OUT], F32)
            onesc_t = cpool.tile([128, 1], F32)
            ones8_t = cpool.tile([8, 1], F32)
            onest_t = cpool.tile([128, 1], F32)
            onesr_t = cpool.tile([1, 128], F32)
            gidx_t = cpool.tile([128, nidx_tot // 16], I16)
            doff_t = cpool.tile([128, nb_tot], F32)
            wsrc_t = cpool.tile([128, nb_tot], F32)
            wts = [cpool.tile([128, OUT], BF16, name=f"wt_s{j}") for j in range(XK)]

            # split the big index loads so the first gathers aren't gated on
            # the full-table DMA
            gsplit = min(nidx_tot // 16, 1024)
            nc.sync.dma_start(gidx_t[:, :gsplit], gidx_d[:, :gsplit])
            if gsplit < nidx_tot // 16:
                nc.sync.dma_start(gidx_t[:, gsplit:], gidx_d[:, gsplit:])
            dsplit = min(nb_tot, 128)
            nc.sync.dma_start(doff_t[:, :dsplit], doff_d[:, :dsplit])
            nc.sync.dma_start(wsrc_t[:, :dsplit], wdeg_d[:, :dsplit])
            if dsplit < nb_tot:
                nc.sync.dma_start(doff_t[:, dsplit:], doff_d[:, dsplit:])
                nc.sync.dma_start(wsrc_t[:, dsplit:], wdeg_d[:, dsplit:])
            nc.sync.dma_start(iota_t[:], iota_d[:])
            nc.sync.dma_start(ident_t[:], ident_d[:])
            nc.sync.dma_start(bt_t[:], bt_d[:])
            nc.sync.dma_start(degi_t[:], degi_d[:])
            nc.sync.dma_start(gm_t[:], gm_d[:])
            nc.sync.dma_start(bb_t[:], bb_d[:])
            nc.sync.dma_start(onesc_t[:], onesc_d[:])
            nc.sync.dma_start(ones8_t[:], ones8_d[:])
            nc.sync.dma_start(onest_t[:], onest_d[:])
            nc.sync.dma_start(onesr_t[:], onesr_d[:])
            for j in range(XK):
                nc.sync.dma_start(wts[j][:], wt[j][:])

            # per-edge src norm: w = rsqrt(deg_out[src]) (pad slots carry 1.0);
            # two pieces so the first masks aren't gated on the full tile
            for c0, c1 in ((0, dsplit), (dsplit, nb_tot)):
                if c0 >= c1:
                    continue
                nc.vector.reciprocal(wsrc_t[:, c0:c1], wsrc_t[:, c0:c1])
                nc.scalar.activation(
                    wsrc_t[:, c0:c1], wsrc_t[:, c0:c1],
                    mybir.ActivationFunctionType.Sqrt,
                )
            # dst norm: rsqrt(max(deg_in, 1)) tile-major [GRP, NG]
            nc.vector.tensor_scalar(
                ndst_t[:], degi_t[:], 1.0, None, op0=mybir.AluOpType.max
            )
            nc.vector.reciprocal(ndst_t[:], ndst_t[:])
            nc.scalar.activation(
                ndst_t[:], ndst_t[:], mybir.ActivationFunctionType.Sqrt
            )

            stats_in = dpool.tile([1, 2 * OUT], F32)
            _aspace = "Local" if cfg.get("NOCC") else "Shared"
            stats_out = dpool.tile([C, 2 * OUT], F32, addr_space=_aspace)

            hrelu_t = hpool.tile([128, NG, OUT], F32)

            # ---- PSUM layout (8 banks x 2KB); accumulation-group zeroing is
            # bank-granular, so every concurrently-open chain gets its own
            # bank: 4x agg (GCHUNK groups in flight) + 1x transpose + 2x out
            # (alternating, WAR-tracked) + 1x BN stats (sum+sq as one chain).
            assert GC == 4
            ps_agg = [
                pagg.tile([128, 2 * OUT], F32, name=f"ps_agg{i}") for i in range(GC)
            ]
            ps_tr = paux.tile([128, 2, OUT], BF16, name="ps_tr")
            ps_out = [
                paux.tile([128, OUT], F32, name=f"ps_out{i}") for i in range(2)
            ]
            ps_stat = pspool.tile([1, 2, OUT], F32, name="ps_stat")

            # ---- stage D: gather x rows + one-hot matmul segmented sum ----
            bmax = max(nb for _, _, nb in batches)
            for bank, t0, nblk in batches:
                Gt = gpool.tile([128, bmax, IN], BF16, tag="G")
                nc.gpsimd.dma_gather(
                    Gt[:, :nblk, :],
                    xb[bank][:],
                    gidx_t[:, t0 * 8 : (t0 + nblk) * 8],
                    nblk * 128,
                    nblk * 128,
                    IN,
                    single_packet=False,
                )
                for j in range(nblk):
                    t = t0 + j
                    b, g, is_start, is_stop = blocks[t]
                    gi = g % GC
                    Mt = mpool.tile([128, GRP], BF16, tag="M")
                    nc.vector.tensor_scalar(
                        Mt[:],
                        iota_t[:],
                        doff_t[:, t : t + 1],
                        wsrc_t[:, t : t + 1],
                        op0=mybir.AluOpType.is_equal,
                        op1=mybir.AluOpType.mult,
                    )
                    nc.tensor.matmul(
                        ps_agg[gi][:],
                        Mt[:],
                        Gt[:, j, :],
                        start=is_start,
                        stop=is_stop,
                    )
                    if not is_stop:
                        continue
                    # ---- group g complete: apply W, then relu/BN partials
                    cagg = capool.tile([128, 2 * OUT], BF16, tag="cagg")
                    nc.scalar.activation(
                        cagg[:], ps_agg[gi][:], mybir.ActivationFunctionType.Copy
                    )
                    for h in range(2):
                        nc.tensor.matmul(
                            ps_tr[:, h, :],
                            cagg[:, h * OUT : (h + 1) * OUT],
                            ident_t[:],
                            is_transpose=True,
                            start=(h == 0),
                            stop=(h == 1),
                        )
                    # one whole-tile copy so the next group's transpose chain
                    # (which pending-zeroes the full bank) WAR-waits on it
                    tagg = capool.tile([128, 2, OUT], BF16, tag="tagg")
                    nc.scalar.activation(
                        tagg[:, :, :], ps_tr[:, :, :], mybir.ActivationFunctionType.Copy
                    )
                    po = ps_out[g % 2]
                    for jj in range(XK):
                        nc.tensor.matmul(
                            po[:],
                            tagg[:, jj, :],
                            wts[jj][:],
                            start=(jj == 0),
                            stop=(jj == XK - 1),
                        )
                    # stage E: relu(out * ndst + b), BN partial sums
                    tmp = epool.tile([128, OUT], F32, tag="etmp")
                    nc.vector.scalar_tensor_tensor(
                        tmp[:],
                        po[:],
                        ndst_t[:, g : g + 1],
                        bt_t[:],
                        op0=mybir.AluOpType.mult,
                        op1=mybir.AluOpType.add,
                    )
                    nc.scalar.activation(
                        hrelu_t[:, g, :], tmp[:], mybir.ActivationFunctionType.Relu
                    )
                    ones = onesc_t if g < NG - 1 else onest_t
                    nc.tensor.matmul(
                        ps_stat[:, 0, :],
                        ones[:],
                        hrelu_t[:, g, :],
                        start=(g == 0),
                        stop=False,
                    )
                    sq = epool.tile([128, OUT], F32, tag="esq")
                    nc.scalar.activation(
                        sq[:], hrelu_t[:, g, :], mybir.ActivationFunctionType.Square
                    )
                    nc.tensor.matmul(
                        ps_stat[:, 1, :],
                        ones[:],
                        sq[:],
                        start=False,
                        stop=(g == NG - 1),
                    )

            # ---- stage F: AllReduce BN stats; build affine S/T tiles ----
            S_t = cpool.tile([128, OUT], F32)
            T_t = cpool.tile([128, OUT], F32)
            st_sb = cpool.tile([1, 2 * OUT], F32)
            nc.scalar.activation(
                st_sb[:], ps_stat[:].rearrange("p a f -> p (a f)"),
                mybir.ActivationFunctionType.Copy,
            )
            nc.sync.dma_start(stats_in[:], st_sb[:])
            if cfg.get("NOCC"):
                stats_out = stats_in  # single-core debug: sums are the totals
                st8 = st_sb
                ones8v = None
            else:
                # AllGather (no 1.875x reduce multiplier) + tiny local
                # ones-matmul reduction beats AllReduce on latency
                nc.gpsimd.collective_compute(
                    "AllGather",
                    mybir.AluOpType.bypass,
                    replica_groups=[list(range(C))],
                    ins=[stats_in[:]],
                    outs=[stats_out[:]],
                )
                st8 = cpool.tile([C, 2 * OUT], F32)
                nc.sync.dma_start(st8[:], stats_out[:])
                ones8v = ones8_t
            st_rb = cpool.tile([1, 2 * OUT], F32)
            if ones8v is None:
                nc.scalar.activation(
                    st_rb[:], st8[:], mybir.ActivationFunctionType.Copy
                )
            else:
                ps_red = ps_stat[:].rearrange("p a f -> p (a f)")
                nc.tensor.matmul(ps_red, ones8v[:], st8[:], start=True, stop=True)
                nc.scalar.activation(
                    st_rb[:], ps_red, mybir.ActivationFunctionType.Copy
                )

            mu = cpool.tile([1, OUT], F32)
            musq = cpool.tile([1, OUT], F32)
            var = cpool.tile([1, OUT], F32)
            srow = cpool.tile([1, OUT], F32)
            trow = cpool.tile([1, OUT], F32)
            inv_n = 1.0 / float(N)
            nc.scalar.activation(
                mu[:], st_rb[:, 0:OUT], mybir.ActivationFunctionType.Copy, scale=inv_n
            )
            nc.scalar.activation(
                musq[:], mu[:], mybir.ActivationFunctionType.Square
            )
            # var + eps = (E[x^2]*inv_n + eps) - mu^2, then rsqrt via
            # reciprocal+sqrt (ACT Rsqrt is banned for accuracy)
            nc.scalar.activation(
                st_rb[:, OUT : 2 * OUT],
                st_rb[:, OUT : 2 * OUT],
                mybir.ActivationFunctionType.Copy,
                scale=inv_n,
                bias=float(cfg["EPS"]),
            )
            nc.vector.tensor_sub(var[:], st_rb[:, OUT : 2 * OUT], musq[:])
            nc.vector.reciprocal(var[:], var[:])
            nc.scalar.activation(
                var[:], var[:], mybir.ActivationFunctionType.Sqrt
            )
            nc.vector.tensor_mul(srow[:], gm_t[:], var[:])
            nc.vector.tensor_mul(trow[:], mu[:], srow[:])
            nc.vector.tensor_sub(trow[:], bb_t[:], trow[:])

            # reuse the (now idle) out banks for the S/T broadcast matmuls
            nc.tensor.matmul(ps_out[0][:], onesr_t[:], srow[:], start=True, stop=True)
            nc.scalar.activation(
                S_t[:], ps_out[0][:], mybir.ActivationFunctionType.Copy
            )
            nc.tensor.matmul(ps_out[1][:], onesr_t[:], trow[:], start=True, stop=True)
            nc.scalar.activation(
                T_t[:], ps_out[1][:], mybir.ActivationFunctionType.Copy
            )

            # ---- stage G: y = hrelu * S + T (in place, S/T broadcast along
            # the group axis), output DMA chunked to overlap with the DVE ----
            ypad_view = ypad_d[:].rearrange("(g p) f -> p g f", p=128)
            GOUT = 13
            for ci, c0 in enumerate(range(0, NG, GOUT)):
                c1 = min(c0 + GOUT, NG)
                S_b = S_t[:].rearrange("p (a f) -> p a f", a=1).to_broadcast(
                    (128, c1 - c0, OUT)
                )
                T_b = T_t[:].rearrange("p (a f) -> p a f", a=1).to_broadcast(
                    (128, c1 - c0, OUT)
                )
                eng = nc.gpsimd if ci % 3 == 2 else nc.vector
                eng.tensor_mul(
                    hrelu_t[:, c0:c1, :], hrelu_t[:, c0:c1, :], S_b
                )
                eng.tensor_add(
                    hrelu_t[:, c0:c1, :], hrelu_t[:, c0:c1, :], T_b
                )
                nc.sync.dma_start(
                    ypad_view[:, c0:c1, :], hrelu_t[:, c0:c1, :]
                )

    nc.compile()
    return nc


def kernel(x, src, dst, W, b, gamma, beta):
    global LAST_RESULTS
    cfg = CFG
    N, E, IN, OUT, C = cfg["N"], cfg["E"], cfg["IN"], cfg["OUT"], cfg["NCORES"]
    GRP = cfg["GRP"]
    assert x.shape == (N, IN) and W.shape == (IN, OUT)
    assert src.shape == (E,) and dst.shape == (E,)

    meta, gidx_cores, dstoff_cores, wdeg_cores = _preprocess(cfg, src, dst)
    NPC, NG, BROWS = meta["NPC"], meta["NG"], meta["BROWS"]
    XK = _ceil_div(IN, 128)
    last_w = NPC - (NG - 1) * GRP

    nc = _build_nc(cfg, meta)

    import ml_dtypes

    x_bf = np.asarray(x, np.float32).astype(ml_dtypes.bfloat16)
    Wn = np.asarray(W, np.float32)

    iota = np.tile(
        np.arange(GRP, dtype=np.float32)[None, :], (128, 1)
    ).astype(ml_dtypes.bfloat16)
    ident = np.eye(128, dtype=np.float32).astype(ml_dtypes.bfloat16)
    bt = np.tile(np.asarray(b, np.float32)[None, :], (128, 1))
    onesc = np.ones((128, 1), np.float32)
    ones8 = np.ones((8, 1), np.float32)
    onest = np.zeros((128, 1), np.float32)
    onest[:last_w] = 1.0
    onesr = np.ones((1, 128), np.float32)
    gm = np.asarray(gamma, np.float32)[None, :]
    bb = np.asarray(beta, np.float32)[None, :]
    xbanks = [
        np.ascontiguousarray(x_bf[q * BROWS : (q + 1) * BROWS])
        for q in range(cfg["NBANKS"])
    ]
    wtiles = [
        np.ascontiguousarray(Wn[j * 128 : (j + 1) * 128, :]).astype(
            ml_dtypes.bfloat16
        )
        for j in range(XK)
    ]

    in_maps = []
    for k in range(C):
        im = {
            "gidx": gidx_cores[k],
            "doff": dstoff_cores[k],
            "wdeg": wdeg_cores[k],
            "degi": _tile_major(
                meta["deg_in"][k * NPC : (k + 1) * NPC], NG, GRP, np.float32(1.0)
            ),
            "bt": bt,
            "iota": iota,
            "ident": ident,
            "gm": gm,
            "bb": bb,
            "onesc": onesc,
            "ones8": ones8,
            "onest": onest,
            "onesr": onesr,
        }
        for q in range(cfg["NBANKS"]):
            im[f"xb{q}"] = xbanks[q]
        for j in range(XK):
            im[f"wt{j}"] = wtiles[j]
        in_maps.append(im)

    if cfg.get("SIM"):
        from concourse.bass_interp import MultiCoreSim

        sim = MultiCoreSim(nc, num_cores=C)
        for k, core_sim in sim.cores.items():
            for name, val in in_maps[k].items():
                core_sim.tensor(name)[:] = val
        sim.simulate()
        y = np.empty((N, OUT), np.float32)
        for k in range(C):
            y[k * NPC : (k + 1) * NPC] = sim.cores[k].tensor("ypad")[:NPC]
        return y

    global LAST_NC, LAST_RUN_S
    LAST_NC = nc
    import time as _time

    _t0 = _time.time()
    res = bass_utils.run_bass_kernel_spmd(
        nc,
        in_maps,
        core_ids=list(range(C)),
        trace=cfg.get("TRACE", False),
    )
    LAST_RUN_S = _time.time() - _t0
    LAST_RESULTS = res

    y = np.empty((N, OUT), np.float32)
    for k in range(C):
        y[k * NPC : (k + 1) * NPC] = res.results[k]["ypad"][:NPC]
    return y


# revision 4
# speedup vs baseline: 1.2632x; 1.0614x over previous
"""GCN block (GraphConv + BatchNorm1d + ReLU) on 8 Trainium2 NeuronCores.

v2 strategy — "gather x, apply W after aggregation":

By linearity, agg[dst] = sum_e norm_src[src_e] * x[src_e] @ W
                       = (sum_e norm_src[src_e] * x[src_e]) @ W.
So instead of computing h = x@W on every shard and AllGather-ing the h table
(collectives dominated the v1 timeline),每 core receives the FULL x (bf16,
row-major) in its own HBM and directly dma_gathers raw x rows for its edges.
No AllGather at all, and gathers start at t=0. x rows are 256 bf16 = 512 B,
which also clears the <512 B small-descriptor DMA penalty that h rows
(128 bf16 = 256 B) pay.

Per core k (owns dst nodes [k*NPC, (k+1)*NPC)):
  1. For each 128-edge block (bucketed by (dst-group, src-bank)), gather
     x[src] rows (bf16, batched dma_gather) and segment-sum with one-hot
     matmuls Mw^T @ G accumulated in PSUM. The one-hot mask is scaled by
     w_e = rsqrt(deg_out[src_e]) (tensor_scalar is_equal*mult), folding
     norm_src into the aggregation.
  2. Per finished dst group: agg_x [128,256] PSUM -> bf16 SBUF -> PE
     transpose -> (aggT_j)^T @ W_j accumulated -> out [128, OUT] PSUM.
  3. relu(out * rsqrt(clip(deg_in,1)) + b); local BN sums via ones-matmul;
     AllReduce BN sums; y = (h - mu) * rsqrt(var+eps) * gamma + beta.

Host-side work is limited to integer index bookkeeping (bucketing edges by
(core, src-bank, dst-group), degree counting) and layout transforms (bf16
cast, int16 gather indices). All floating-point math runs on device.

Edges are bucketed by src bank (4 banks of N/4 rows) because dma_gather
indices are int16 (< 32768). Bucket sizes are padded to a structure shared
by all 8 cores so a single SPMD NEFF serves every core; pad slots gather row
0 of the bank and carry a dst offset of 255 -> their one-hot column is all
zeros, so they contribute exactly 0. Banks 0 and 3 are padded to >= 1 block
per group so every group starts in bank 0 and stops in bank 3 (keeps the
BN-stat accumulation chain's start first / stop last in program order).
"""
import math
import os
import sys

sys.path.insert(0, "/opt/trn_rl_repo")

import numpy as np

import concourse.bacc as bacc
import concourse.bass as bass
import concourse.mybir as mybir
import concourse.tile as tile
from concourse import bass_utils

F32 = mybir.dt.float32
BF16 = mybir.dt.bfloat16
I16 = mybir.dt.int16

CFG = dict(
    N=100000,
    E=1600000,
    IN=256,
    OUT=128,
    NCORES=8,
    GRP=128,          # dst nodes per segment group (= psum partition dim)
    NBANKS=4,         # src banks (bank rows must stay < 32768 for int16 idx)
    GCHUNK=4,         # dst groups in flight (1 PSUM bank per open accum chain)
    BATCH_BLOCKS=40,  # gather batch size in 128-edge blocks
    EPS=1e-5,
    TRACE=False,
)

LAST_RESULTS = None  # set by kernel() for test harness introspection
LAST_NC = None
LAST_RUN_S = None


def _ceil_div(a, b):
    return (a + b - 1) // b


def _wrap16(idx, ncols):
    """int16 idx list -> [128, ncols] tile: idx i at [i%16, i//16], replicated
    8x across the 16-partition groups (one copy per GpSimd Q7 core)."""
    n = idx.shape[0]
    assert n == ncols * 16
    w = np.ascontiguousarray(idx.reshape(ncols, 16).T)
    return np.tile(w, (8, 1))


def _preprocess(cfg, src, dst):
    """Bucket edges by (owner core, src bank, dst group); build per-core
    gather-index / dst-offset / src-degree arrays and the shared block
    structure."""
    N, E = cfg["N"], cfg["E"]
    C, NBANKS, GRP = cfg["NCORES"], cfg["NBANKS"], cfg["GRP"]
    NPC = N // C
    NG = _ceil_div(NPC, GRP)
    assert N % NBANKS == 0
    BROWS = N // NBANKS            # rows per x bank (gather source table)
    assert BROWS < 32768

    src = src.astype(np.int64)
    dst = dst.astype(np.int64)
    deg_out = np.bincount(src, minlength=N).astype(np.float32)
    deg_in = np.bincount(dst, minlength=N).astype(np.float32)

    owner = dst // NPC
    bank = src // BROWS
    grp = (dst % NPC) // GRP
    key = (owner * NBANKS + bank) * NG + grp
    order = np.argsort(key, kind="stable")
    s_src = src[order]
    s_dst = dst[order]
    s_key = key[order]

    counts = np.bincount(key, minlength=C * NBANKS * NG).reshape(C, NBANKS, NG)
    # bucket capacity: exact max over cores (shared SPMD structure); >= 1 in
    # banks 0/3 so every group has a first (bank-0) and last (bank-3) matmul
    P = counts.max(axis=0)  # [NBANKS, NG]
    P[0] = np.maximum(P[0], 1)
    P[NBANKS - 1] = np.maximum(P[NBANKS - 1], 1)

    # stream order: group-chunks outer, banks inner; a group's PSUM slot is
    # live across all banks of its chunk (accumulated with start/stop).
    # Segment (chunk, bank) = that chunk's buckets concatenated, padded to a
    # multiple of 128; 128-edge blocks may straddle bucket (group) boundaries
    # -> one matmul per (block, overlapped group).
    GC = cfg["GCHUNK"]
    chunks = [list(range(c, min(c + GC, NG))) for c in range(0, NG, GC)]
    run_off = np.zeros((NBANKS, NG), np.int64)
    segments = []  # (bank, seg_start_slot, seg_nblocks)
    pos = 0
    for ch in chunks:
        for b in range(NBANKS):
            seg0 = pos
            for g in ch:
                run_off[b, g] = pos
                pos += P[b, g]
            pos = ((pos + 127) // 128) * 128  # segment tail pad
            segments.append((b, seg0, (pos - seg0) // 128))
    nidx_tot = pos
    nb_tot = nidx_tot // 128

    # per block: list of matmuls (mm_col, group); per group: first/last mm id
    block_mms = [[] for _ in range(nb_tot)]
    n_mm = 0
    mm_of_group = {}
    for ch in chunks:
        for b in range(NBANKS):
            for g in ch:
                o0, o1 = run_off[b, g], run_off[b, g] + P[b, g]
                for t in range(o0 // 128, (o1 - 1) // 128 + 1):
                    block_mms[t].append((n_mm, g))
                    mm_of_group.setdefault(g, []).append(n_mm)
                    n_mm += 1
    mm_flags = {}
    for g, mms in mm_of_group.items():
        for m in mms:
            mm_flags[m] = (m == mms[0], m == mms[-1])

    # boundaries of each (k, b, g) bucket in the sorted edge stream
    bkeys = (np.arange(C)[:, None, None] * NBANKS + np.arange(NBANKS)[None, :, None]) * NG + np.arange(NG)[None, None, :]
    starts = np.searchsorted(s_key, bkeys.ravel()).reshape(C, NBANKS, NG)
    ends = np.searchsorted(s_key, bkeys.ravel(), side="right").reshape(C, NBANKS, NG)

    gidx_cores = []
    dstoff_cores = []
    wdeg_cores = []
    for k in range(C):
        gidx = np.zeros(nidx_tot, np.int16)
        doff = np.full((n_mm, 128), 255.0, np.float32)
        wdeg = np.ones((n_mm, 128), np.float32)
        # fill gather indices per bucket (pad slots keep row 0)
        for b in range(NBANKS):
            for g in range(NG):
                s, e = starts[k, b, g], ends[k, b, g]
                cnt = e - s
                if cnt == 0:
                    continue
                p0 = run_off[b, g]
                gidx[p0 : p0 + cnt] = (s_src[s:e] % BROWS).astype(np.int16)
        # fill per-matmul mask columns: rows = this core's real edges of the
        # matmul's group that fall inside the block's 128-slot window
        for ch in chunks:
            for b in range(NBANKS):
                for g in ch:
                    s, e = starts[k, b, g], ends[k, b, g]
                    cnt = e - s
                    o0 = run_off[b, g]
                    bucket_mms = [
                        (m, t)
                        for t in range(o0 // 128, (o0 + P[b, g] - 1) // 128 + 1)
                        for (m, gg) in block_mms[t]
                        if gg == g
                    ]
                    if cnt == 0:
                        continue
                    dvals = ((s_dst[s:e] % NPC) - g * GRP).astype(np.float32)
                    wvals = deg_out[s_src[s:e]]
                    for m, t in bucket_mms:
                        w0 = t * 128
                        lo = max(o0, w0)
                        hi = min(o0 + cnt, w0 + 128)
                        if lo >= hi:
                            continue
                        rows = np.arange(lo - w0, hi - w0)
                        doff[m, rows] = dvals[lo - o0 : hi - o0]
                        wdeg[m, rows] = wvals[lo - o0 : hi - o0]
        gidx_cores.append(_wrap16(gidx, nidx_tot // 16))
        dstoff_cores.append(np.ascontiguousarray(doff.T))
        wdeg_cores.append(np.ascontiguousarray(wdeg.T))

    # gather batches: split segments longer than BATCH_BLOCKS
    batches = []  # (bank, first_block, n_blocks)
    for b, seg0, nblk in segments:
        t = seg0 // 128
        left = nblk
        while left > 0:
            n = min(left, cfg["BATCH_BLOCKS"])
            batches.append((b, t, n))
            t += n
            left -= n

    meta = dict(
        NPC=NPC,
        NG=NG,
        BROWS=BROWS,
        nidx_tot=nidx_tot,
        nb_tot=nb_tot,
        n_mm=n_mm,
        block_mms=block_mms,
        mm_flags=mm_flags,
        batches=batches,
        deg_in=deg_in,
    )
    return meta, gidx_cores, dstoff_cores, wdeg_cores


def _tile_major(vec, NG, GRP, pad_val):
    """[NPC] -> [GRP, NG]: entry (p, m) = vec[m*GRP + p], padded."""
    out = np.full((NG * GRP,), pad_val, vec.dtype)
    out[: vec.shape[0]] = vec
    return np.ascontiguousarray(out.reshape(NG, GRP).T)


def _build_nc(cfg, meta):
    N, IN, OUT, C = cfg["N"], cfg["IN"], cfg["OUT"], cfg["NCORES"]
    GRP, NBANKS, GC = cfg["GRP"], cfg["NBANKS"], cfg["GCHUNK"]
    NPC, NG, BROWS = meta["NPC"], meta["NG"], meta["BROWS"]
    nidx_tot, nb_tot = meta["nidx_tot"], meta["nb_tot"]
    n_mm = meta["n_mm"]
    block_mms, mm_flags = meta["block_mms"], meta["mm_flags"]
    batches = meta["batches"]
    XK = _ceil_div(IN, 128)
    assert OUT == 128 and GRP == 128 and XK == 2
    last_w = NPC - (NG - 1) * GRP  # valid rows in the last group

    nc = bacc.Bacc(
        "TRN2", target_bir_lowering=False, debug=False, num_devices=C
    )

    # ---- external inputs ----
    xb = [
        nc.dram_tensor(f"xb{q}", [BROWS, IN], BF16, kind="ExternalInput")
        for q in range(NBANKS)
    ]
    wt = [
        nc.dram_tensor(f"wt{j}", [128, OUT], BF16, kind="ExternalInput")
        for j in range(XK)
    ]
    gidx_d = nc.dram_tensor("gidx", [128, nidx_tot // 16], I16, kind="ExternalInput")
    doff_d = nc.dram_tensor("doff", [128, n_mm], F32, kind="ExternalInput")
    wdeg_d = nc.dram_tensor("wdeg", [128, n_mm], F32, kind="ExternalInput")
    degi_d = nc.dram_tensor("degi", [128, NG], F32, kind="ExternalInput")
    bt_d = nc.dram_tensor("bt", [128, OUT], F32, kind="ExternalInput")
    iota_d = nc.dram_tensor("iota", [128, GRP], BF16, kind="ExternalInput")
    ident_d = nc.dram_tensor("ident", [128, 128], BF16, kind="ExternalInput")
    gm_d = nc.dram_tensor("gm", [1, OUT], F32, kind="ExternalInput")
    bb_d = nc.dram_tensor("bb", [1, OUT], F32, kind="ExternalInput")
    onesc_d = nc.dram_tensor("onesc", [128, 1], F32, kind="ExternalInput")
    ones8_d = nc.dram_tensor("ones8", [8, 1], F32, kind="ExternalInput")
    onest_d = nc.dram_tensor("onest", [128, 1], F32, kind="ExternalInput")
    onesr_d = nc.dram_tensor("onesr", [1, 128], F32, kind="ExternalInput")

    ypad_d = nc.dram_tensor("ypad", [NG * GRP, OUT], F32, kind="ExternalOutput")

    with tile.TileContext(nc) as tc:
        with (
            tc.tile_pool(name="const", bufs=1) as cpool,
            tc.tile_pool(name="dram", bufs=1, space="DRAM") as dpool,
            tc.tile_pool(name="hrelu", bufs=1) as hpool,
            tc.tile_pool(name="gath", bufs=4) as gpool,
            tc.tile_pool(name="mpool", bufs=10) as mpool,
            tc.tile_pool(name="capool", bufs=6) as capool,
            tc.tile_pool(name="etmp", bufs=6) as epool,
            tc.tile_pool(name="gtmp", bufs=4) as gpool2,
            tc.tile_pool(name="psagg", bufs=1, space="PSUM") as pagg,
            tc.tile_pool(name="psaux", bufs=1, space="PSUM") as paux,
            tc.tile_pool(name="pstat", bufs=1, space="PSUM") as pspool,
        ):
            # ---- constants / small tiles ----
            iota_t = cpool.tile([128, GRP], BF16)
            ident_t = cpool.tile([128, 128], BF16)
            bt_t = cpool.tile([128, OUT], F32)
            degi_t = cpool.tile([128, NG], F32)
            ndst_t = cpool.tile([128, NG], F32)
            gm_t = cpool.tile([1, OUT], F32)
            bb_t = cpool.tile([1, OUT], F32)
            onesc_t = cpool.tile([128, 1], F32)
            ones8_t = cpool.tile([8, 1], F32)
            onest_t = cpool.tile([128, 1], F32)
            onesr_t = cpool.tile([1, 128], F32)
            gidx_t = cpool.tile([128, nidx_tot // 16], I16)
            doff_t = cpool.tile([128, n_mm], F32)
            wsrc_t = cpool.tile([128, n_mm], F32)
            wts = [cpool.tile([128, OUT], BF16, name=f"wt_s{j}") for j in range(XK)]

            # split the big index loads so the first gathers aren't gated on
            # the full-table DMA
            gsplit = min(nidx_tot // 16, 256)
            nc.sync.dma_start(gidx_t[:, :gsplit], gidx_d[:, :gsplit])
            if gsplit < nidx_tot // 16:
                nc.sync.dma_start(gidx_t[:, gsplit:], gidx_d[:, gsplit:])
            dsplit = min(n_mm, 128)
            nc.sync.dma_start(doff_t[:, :dsplit], doff_d[:, :dsplit])
            nc.sync.dma_start(wsrc_t[:, :dsplit], wdeg_d[:, :dsplit])
            if dsplit < n_mm:
                nc.sync.dma_start(doff_t[:, dsplit:], doff_d[:, dsplit:])
                nc.sync.dma_start(wsrc_t[:, dsplit:], wdeg_d[:, dsplit:])
            nc.sync.dma_start(iota_t[:], iota_d[:])
            nc.sync.dma_start(ident_t[:], ident_d[:])
            nc.sync.dma_start(bt_t[:], bt_d[:])
            nc.sync.dma_start(degi_t[:], degi_d[:])
            nc.sync.dma_start(gm_t[:], gm_d[:])
            nc.sync.dma_start(bb_t[:], bb_d[:])
            nc.sync.dma_start(onesc_t[:], onesc_d[:])
            nc.sync.dma_start(ones8_t[:], ones8_d[:])
            nc.sync.dma_start(onest_t[:], onest_d[:])
            nc.sync.dma_start(onesr_t[:], onesr_d[:])
            for j in range(XK):
                nc.sync.dma_start(wts[j][:], wt[j][:])

            # per-edge src norm: w = rsqrt(deg_out[src]) (pad slots carry 1.0);
            # two pieces so the first masks aren't gated on the full tile
            for c0, c1 in ((0, dsplit), (dsplit, n_mm)):
                if c0 >= c1:
                    continue
                nc.vector.reciprocal(wsrc_t[:, c0:c1], wsrc_t[:, c0:c1])
                nc.scalar.activation(
                    wsrc_t[:, c0:c1], wsrc_t[:, c0:c1],
                    mybir.ActivationFunctionType.Sqrt,
                )
            # dst norm: rsqrt(max(deg_in, 1)) tile-major [GRP, NG]
            nc.vector.tensor_scalar(
                ndst_t[:], degi_t[:], 1.0, None, op0=mybir.AluOpType.max
            )
            nc.vector.reciprocal(ndst_t[:], ndst_t[:])
            nc.scalar.activation(
                ndst_t[:], ndst_t[:], mybir.ActivationFunctionType.Sqrt
            )

            stats_in = dpool.tile([1, 2 * OUT], F32)
            _aspace = "Local" if cfg.get("NOCC") else "Shared"
            stats_out = dpool.tile([C, 2 * OUT], F32, addr_space=_aspace)

            hrelu_t = hpool.tile([128, NG, OUT], F32)

            # ---- PSUM layout (8 banks x 2KB); accumulation-group zeroing is
            # bank-granular, so every concurrently-open chain gets its own
            # bank: 4x agg (GCHUNK groups in flight) + 1x transpose + 2x out
            # (alternating, WAR-tracked) + 1x BN stats (sum+sq as one chain).
            assert GC == 4
            ps_agg = [
                pagg.tile([128, 2 * OUT], F32, name=f"ps_agg{i}") for i in range(GC)
            ]
            ps_tr = paux.tile([128, 2, OUT], BF16, name="ps_tr")
            ps_out = [
                paux.tile([128, OUT], F32, name=f"ps_out{i}") for i in range(2)
            ]
            ps_stat = pspool.tile([1, 2, OUT], F32, name="ps_stat")

            # ---- stage D: gather x rows + one-hot matmul segmented sum ----
            def _finish_group(g):
                """Group g's PSUM agg is complete: apply W, relu, BN partials."""
                gi = g % GC
                cagg = capool.tile([128, 2 * OUT], BF16, tag="cagg")
                nc.scalar.activation(
                    cagg[:], ps_agg[gi][:], mybir.ActivationFunctionType.Copy
                )
                for h in range(2):
                    nc.tensor.matmul(
                        ps_tr[:, h, :],
                        cagg[:, h * OUT : (h + 1) * OUT],
                        ident_t[:],
                        is_transpose=True,
                        start=(h == 0),
                        stop=(h == 1),
                    )
                # one whole-tile copy so the next group's transpose chain
                # (which pending-zeroes the full bank) WAR-waits on it
                tagg = capool.tile([128, 2, OUT], BF16, tag="tagg")
                nc.scalar.activation(
                    tagg[:, :, :], ps_tr[:, :, :],
                    mybir.ActivationFunctionType.Copy,
                )
                po = ps_out[g % 2]
                for jj in range(XK):
                    nc.tensor.matmul(
                        po[:],
                        tagg[:, jj, :],
                        wts[jj][:],
                        start=(jj == 0),
                        stop=(jj == XK - 1),
                    )
                # stage E: relu(out * ndst + b), BN partial sums
                tmp = epool.tile([128, OUT], F32, tag="etmp")
                nc.vector.scalar_tensor_tensor(
                    tmp[:],
                    po[:],
                    ndst_t[:, g : g + 1],
                    bt_t[:],
                    op0=mybir.AluOpType.mult,
                    op1=mybir.AluOpType.add,
                )
                nc.scalar.activation(
                    hrelu_t[:, g, :], tmp[:], mybir.ActivationFunctionType.Relu
                )
                ones = onesc_t if g < NG - 1 else onest_t
                nc.tensor.matmul(
                    ps_stat[:, 0, :],
                    ones[:],
                    hrelu_t[:, g, :],
                    start=(g == 0),
                    stop=False,
                )
                sq = epool.tile([128, OUT], F32, tag="esq")
                nc.scalar.activation(
                    sq[:], hrelu_t[:, g, :], mybir.ActivationFunctionType.Square
                )
                nc.tensor.matmul(
                    ps_stat[:, 1, :],
                    ones[:],
                    sq[:],
                    start=False,
                    stop=(g == NG - 1),
                )

            bmax = max(nb for _, _, nb in batches)
            for bank, t0, nblk in batches:
                Gt = gpool.tile([128, bmax, IN], BF16, tag="G")
                nc.gpsimd.dma_gather(
                    Gt[:, :nblk, :],
                    xb[bank][:],
                    gidx_t[:, t0 * 8 : (t0 + nblk) * 8],
                    nblk * 128,
                    nblk * 128,
                    IN,
                    single_packet=False,
                )
                for j in range(nblk):
                    t = t0 + j
                    for m, g in block_mms[t]:
                      is_start, is_stop = mm_flags[m]
                      gi = g % GC
                      Mt = mpool.tile([128, GRP], BF16, tag="M")
                      nc.vector.tensor_scalar(
                          Mt[:],
                          iota_t[:],
                          doff_t[:, m : m + 1],
                          wsrc_t[:, m : m + 1],
                          op0=mybir.AluOpType.is_equal,
                          op1=mybir.AluOpType.mult,
                      )
                      nc.tensor.matmul(
                          ps_agg[gi][:],
                          Mt[:],
                          Gt[:, j, :],
                          start=is_start,
                          stop=is_stop,
                      )
                      if is_stop:
                          _finish_group(g)

            # ---- stage F: AllReduce BN stats; build affine S/T tiles ----
            S_t = cpool.tile([128, OUT], F32)
            T_t = cpool.tile([128, OUT], F32)
            st_sb = cpool.tile([1, 2 * OUT], F32)
            nc.scalar.activation(
                st_sb[:], ps_stat[:].rearrange("p a f -> p (a f)"),
                mybir.ActivationFunctionType.Copy,
            )
            nc.sync.dma_start(stats_in[:], st_sb[:])
            if cfg.get("NOCC"):
                stats_out = stats_in  # single-core debug: sums are the totals
                st8 = st_sb
                ones8v = None
            else:
                # AllGather (no 1.875x reduce multiplier) + tiny local
                # ones-matmul reduction beats AllReduce on latency
                nc.gpsimd.collective_compute(
                    "AllGather",
                    mybir.AluOpType.bypass,
                    replica_groups=[list(range(C))],
                    ins=[stats_in[:]],
                    outs=[stats_out[:]],
                )
                st8 = cpool.tile([C, 2 * OUT], F32)
                nc.sync.dma_start(st8[:], stats_out[:])
                ones8v = ones8_t
            st_rb = cpool.tile([1, 2 * OUT], F32)
            if ones8v is None:
                nc.scalar.activation(
                    st_rb[:], st8[:], mybir.ActivationFunctionType.Copy
                )
            else:
                ps_red = ps_stat[:].rearrange("p a f -> p (a f)")
                nc.tensor.matmul(ps_red, ones8v[:], st8[:], start=True, stop=True)
                nc.scalar.activation(
                    st_rb[:], ps_red, mybir.ActivationFunctionType.Copy
                )

            mu = cpool.tile([1, OUT], F32)
            musq = cpool.tile([1, OUT], F32)
            var = cpool.tile([1, OUT], F32)
            srow = cpool.tile([1, OUT], F32)
            trow = cpool.tile([1, OUT], F32)
            inv_n = 1.0 / float(N)
            nc.scalar.activation(
                mu[:], st_rb[:, 0:OUT], mybir.ActivationFunctionType.Copy, scale=inv_n
            )
            nc.scalar.activation(
                musq[:], mu[:], mybir.ActivationFunctionType.Square
            )
            # var + eps = (E[x^2]*inv_n + eps) - mu^2, then rsqrt via
            # reciprocal+sqrt (ACT Rsqrt is banned for accuracy)
            nc.scalar.activation(
                st_rb[:, OUT : 2 * OUT],
                st_rb[:, OUT : 2 * OUT],
                mybir.ActivationFunctionType.Copy,
                scale=inv_n,
                bias=float(cfg["EPS"]),
            )
            nc.vector.tensor_sub(var[:], st_rb[:, OUT : 2 * OUT], musq[:])
            nc.vector.reciprocal(var[:], var[:])
            nc.scalar.activation(
                var[:], var[:], mybir.ActivationFunctionType.Sqrt
            )
            nc.vector.tensor_mul(srow[:], gm_t[:], var[:])
            nc.vector.tensor_mul(trow[:], mu[:], srow[:])
            nc.vector.tensor_sub(trow[:], bb_t[:], trow[:])

            # reuse the (now idle) out banks for the S/T broadcast matmuls
            nc.tensor.matmul(ps_out[0][:], onesr_t[:], srow[:], start=True, stop=True)
            nc.scalar.activation(
                S_t[:], ps_out[0][:], mybir.ActivationFunctionType.Copy
            )
            nc.tensor.matmul(ps_out[1][:], onesr_t[:], trow[:], start=True, stop=True)
            nc.scalar.activation(
                T_t[:], ps_out[1][:], mybir.ActivationFunctionType.Copy
            )

            # ---- stage G: y = hrelu * S + T (in place, S/T broadcast along
            # the group axis), output DMA chunked to overlap with the DVE ----
            ypad_view = ypad_d[:].rearrange("(g p) f -> p g f", p=128)
            GOUT = 13
            for ci, c0 in enumerate(range(0, NG, GOUT)):
                c1 = min(c0 + GOUT, NG)
                S_b = S_t[:].rearrange("p (a f) -> p a f", a=1).to_broadcast(
                    (128, c1 - c0, OUT)
                )
                T_b = T_t[:].rearrange("p (a f) -> p a f", a=1).to_broadcast(
                    (128, c1 - c0, OUT)
                )
                eng = nc.gpsimd if ci % 2 == 1 else nc.vector
                eng.tensor_mul(
                    hrelu_t[:, c0:c1, :], hrelu_t[:, c0:c1, :], S_b
                )
                eng.tensor_add(
                    hrelu_t[:, c0:c1, :], hrelu_t[:, c0:c1, :], T_b
                )
                nc.sync.dma_start(
                    ypad_view[:, c0:c1, :], hrelu_t[:, c0:c1, :]
                )

    nc.compile()
    return nc


def kernel(x, src, dst, W, b, gamma, beta):
    global LAST_RESULTS
    cfg = CFG
    N, E, IN, OUT, C = cfg["N"], cfg["E"], cfg["IN"], cfg["OUT"], cfg["NCORES"]
    GRP = cfg["GRP"]
    assert x.shape == (N, IN) and W.shape == (IN, OUT)
    assert src.shape == (E,) and dst.shape == (E,)

    meta, gidx_cores, dstoff_cores, wdeg_cores = _preprocess(cfg, src, dst)
    NPC, NG, BROWS = meta["NPC"], meta["NG"], meta["BROWS"]
    XK = _ceil_div(IN, 128)
    last_w = NPC - (NG - 1) * GRP

    nc = _build_nc(cfg, meta)

    import ml_dtypes

    x_bf = np.asarray(x, np.float32).astype(ml_dtypes.bfloat16)
    Wn = np.asarray(W, np.float32)

    iota = np.tile(
        np.arange(GRP, dtype=np.float32)[None, :], (128, 1)
    ).astype(ml_dtypes.bfloat16)
    ident = np.eye(128, dtype=np.float32).astype(ml_dtypes.bfloat16)
    bt = np.tile(np.asarray(b, np.float32)[None, :], (128, 1))
    onesc = np.ones((128, 1), np.float32)
    ones8 = np.ones((8, 1), np.float32)
    onest = np.zeros((128, 1), np.float32)
    onest[:last_w] = 1.0
    onesr = np.ones((1, 128), np.float32)
    gm = np.asarray(gamma, np.float32)[None, :]
    bb = np.asarray(beta, np.float32)[None, :]
    xbanks = [
        np.ascontiguousarray(x_bf[q * BROWS : (q + 1) * BROWS])
        for q in range(cfg["NBANKS"])
    ]
    wtiles = [
        np.ascontiguousarray(Wn[j * 128 : (j + 1) * 128, :]).astype(
            ml_dtypes.bfloat16
        )
        for j in range(XK)
    ]

    in_maps = []
    for k in range(C):
        im = {
            "gidx": gidx_cores[k],
            "doff": dstoff_cores[k],
            "wdeg": wdeg_cores[k],
            "degi": _tile_major(
                meta["deg_in"][k * NPC : (k + 1) * NPC], NG, GRP, np.float32(1.0)
            ),
            "bt": bt,
            "iota": iota,
            "ident": ident,
            "gm": gm,
            "bb": bb,
            "onesc": onesc,
            "ones8": ones8,
            "onest": onest,
            "onesr": onesr,
        }
        for q in range(cfg["NBANKS"]):
            im[f"xb{q}"] = xbanks[q]
        for j in range(XK):
            im[f"wt{j}"] = wtiles[j]
        in_maps.append(im)

    if cfg.get("SIM"):
        from concourse.bass_interp import MultiCoreSim

        sim = MultiCoreSim(nc, num_cores=C)
        for k, core_sim in sim.cores.items():
            for name, val in in_maps[k].items():
                core_sim.tensor(name)[:] = val
        sim.simulate()
        y = np.empty((N, OUT), np.float32)
        for k in range(C):
            y[k * NPC : (k + 1) * NPC] = sim.cores[k].tensor("ypad")[:NPC]
        return y

    global LAST_NC, LAST_RUN_S
    LAST_NC = nc
    import time as _time

    _t0 = _time.time()
    res = bass_utils.run_bass_kernel_spmd(
        nc,
        in_maps,
        core_ids=list(range(C)),
        trace=cfg.get("TRACE", False),
    )
    LAST_RUN_S = _time.time() - _t0
    LAST_RESULTS = res

    y = np.empty((N, OUT), np.float32)
    for k in range(C):
        y[k * NPC : (k + 1) * NPC] = res.results[k]["ypad"][:NPC]
    return y


# revision 5
# speedup vs baseline: 1.2788x; 1.0123x over previous
"""GCN block (GraphConv + BatchNorm1d + ReLU) on 8 Trainium2 NeuronCores.

v2 strategy — "gather x, apply W after aggregation":

By linearity, agg[dst] = sum_e norm_src[src_e] * x[src_e] @ W
                       = (sum_e norm_src[src_e] * x[src_e]) @ W.
So instead of computing h = x@W on every shard and AllGather-ing the h table
(collectives dominated the v1 timeline),每 core receives the FULL x (bf16,
row-major) in its own HBM and directly dma_gathers raw x rows for its edges.
No AllGather at all, and gathers start at t=0. x rows are 256 bf16 = 512 B,
which also clears the <512 B small-descriptor DMA penalty that h rows
(128 bf16 = 256 B) pay.

Per core k (owns dst nodes [k*NPC, (k+1)*NPC)):
  1. For each 128-edge block (bucketed by (dst-group, src-bank)), gather
     x[src] rows (bf16, batched dma_gather) and segment-sum with one-hot
     matmuls Mw^T @ G accumulated in PSUM. The one-hot mask is scaled by
     w_e = rsqrt(deg_out[src_e]) (tensor_scalar is_equal*mult), folding
     norm_src into the aggregation.
  2. Per finished dst group: agg_x [128,256] PSUM -> bf16 SBUF -> PE
     transpose -> (aggT_j)^T @ W_j accumulated -> out [128, OUT] PSUM.
  3. relu(out * rsqrt(clip(deg_in,1)) + b); local BN sums via ones-matmul;
     AllReduce BN sums; y = (h - mu) * rsqrt(var+eps) * gamma + beta.

Host-side work is limited to integer index bookkeeping (bucketing edges by
(core, src-bank, dst-group), degree counting) and layout transforms (bf16
cast, int16 gather indices). All floating-point math runs on device.

Edges are bucketed by src bank (4 banks of N/4 rows) because dma_gather
indices are int16 (< 32768). Bucket sizes are padded to a structure shared
by all 8 cores so a single SPMD NEFF serves every core; pad slots gather row
0 of the bank and carry a dst offset of 255 -> their one-hot column is all
zeros, so they contribute exactly 0. Banks 0 and 3 are padded to >= 1 block
per group so every group starts in bank 0 and stops in bank 3 (keeps the
BN-stat accumulation chain's start first / stop last in program order).
"""
import math
import os
import sys

sys.path.insert(0, "/opt/trn_rl_repo")

import numpy as np

import concourse.bacc as bacc
import concourse.bass as bass
import concourse.mybir as mybir
import concourse.tile as tile
from concourse import bass_utils

F32 = mybir.dt.float32
BF16 = mybir.dt.bfloat16
I16 = mybir.dt.int16

CFG = dict(
    N=100000,
    E=1600000,
    IN=256,
    OUT=128,
    NCORES=8,
    GRP=128,          # dst nodes per segment group (= psum partition dim)
    NBANKS=4,         # src banks (bank rows must stay < 32768 for int16 idx)
    GCHUNK=4,         # dst groups in flight (1 PSUM bank per open accum chain)
    BATCH_BLOCKS=40,  # gather batch size in 128-edge blocks
    EPS=1e-5,
    TRACE=False,
)

LAST_RESULTS = None  # set by kernel() for test harness introspection
LAST_NC = None
LAST_RUN_S = None


def _ceil_div(a, b):
    return (a + b - 1) // b


def _wrap16(idx, ncols):
    """int16 idx list -> [128, ncols] tile: idx i at [i%16, i//16], replicated
    8x across the 16-partition groups (one copy per GpSimd Q7 core)."""
    n = idx.shape[0]
    assert n == ncols * 16
    w = np.ascontiguousarray(idx.reshape(ncols, 16).T)
    return np.tile(w, (8, 1))


def _preprocess(cfg, src, dst):
    """Bucket edges by (owner core, src bank, dst group); build per-core
    gather-index / dst-offset / src-degree arrays and the shared block
    structure."""
    N, E = cfg["N"], cfg["E"]
    C, NBANKS, GRP = cfg["NCORES"], cfg["NBANKS"], cfg["GRP"]
    NPC = N // C
    NG = _ceil_div(NPC, GRP)
    assert N % NBANKS == 0
    BROWS = N // NBANKS            # rows per x bank (gather source table)
    assert BROWS < 32768

    src = src.astype(np.int64)
    dst = dst.astype(np.int64)
    deg_out = np.bincount(src, minlength=N).astype(np.float32)
    deg_in = np.bincount(dst, minlength=N).astype(np.float32)

    owner = dst // NPC
    bank = src // BROWS
    grp = (dst % NPC) // GRP
    key = (owner * NBANKS + bank) * NG + grp
    order = np.argsort(key, kind="stable")
    s_src = src[order]
    s_dst = dst[order]
    s_key = key[order]

    counts = np.bincount(key, minlength=C * NBANKS * NG).reshape(C, NBANKS, NG)
    # bucket capacity: exact max over cores (shared SPMD structure); >= 1 in
    # banks 0/3 so every group has a first (bank-0) and last (bank-3) matmul
    P = counts.max(axis=0)  # [NBANKS, NG]
    P[0] = np.maximum(P[0], 1)
    P[NBANKS - 1] = np.maximum(P[NBANKS - 1], 1)

    # stream order: group-chunks outer, banks inner; a group's PSUM slot is
    # live across all banks of its chunk (accumulated with start/stop).
    # Segment (chunk, bank) = that chunk's buckets concatenated, padded to a
    # multiple of 128; 128-edge blocks may straddle bucket (group) boundaries
    # -> one matmul per (block, overlapped group).
    GC = cfg["GCHUNK"]
    chunks = [list(range(c, min(c + GC, NG))) for c in range(0, NG, GC)]
    run_off = np.zeros((NBANKS, NG), np.int64)
    segments = []  # (bank, seg_start_slot, seg_nblocks)
    pos = 0
    for ch in chunks:
        for b in range(NBANKS):
            seg0 = pos
            for g in ch:
                run_off[b, g] = pos
                pos += P[b, g]
            pos = ((pos + 127) // 128) * 128  # segment tail pad
            segments.append((b, seg0, (pos - seg0) // 128))
    nidx_tot = pos
    nb_tot = nidx_tot // 128

    # per block: list of matmuls (mm_col, group); per group: first/last mm id
    block_mms = [[] for _ in range(nb_tot)]
    n_mm = 0
    mm_of_group = {}
    for ch in chunks:
        for b in range(NBANKS):
            for g in ch:
                o0, o1 = run_off[b, g], run_off[b, g] + P[b, g]
                for t in range(o0 // 128, (o1 - 1) // 128 + 1):
                    block_mms[t].append((n_mm, g))
                    mm_of_group.setdefault(g, []).append(n_mm)
                    n_mm += 1
    mm_flags = {}
    for g, mms in mm_of_group.items():
        for m in mms:
            mm_flags[m] = (m == mms[0], m == mms[-1])

    # boundaries of each (k, b, g) bucket in the sorted edge stream
    bkeys = (np.arange(C)[:, None, None] * NBANKS + np.arange(NBANKS)[None, :, None]) * NG + np.arange(NG)[None, None, :]
    starts = np.searchsorted(s_key, bkeys.ravel()).reshape(C, NBANKS, NG)
    ends = np.searchsorted(s_key, bkeys.ravel(), side="right").reshape(C, NBANKS, NG)

    gidx_cores = []
    dstoff_cores = []
    wdeg_cores = []
    for k in range(C):
        gidx = np.zeros(nidx_tot, np.int16)
        doff = np.full((n_mm, 128), 255.0, np.float32)
        wdeg = np.ones((n_mm, 128), np.float32)
        # fill gather indices per bucket (pad slots keep row 0)
        for b in range(NBANKS):
            for g in range(NG):
                s, e = starts[k, b, g], ends[k, b, g]
                cnt = e - s
                if cnt == 0:
                    continue
                p0 = run_off[b, g]
                gidx[p0 : p0 + cnt] = (s_src[s:e] % BROWS).astype(np.int16)
        # fill per-matmul mask columns: rows = this core's real edges of the
        # matmul's group that fall inside the block's 128-slot window
        for ch in chunks:
            for b in range(NBANKS):
                for g in ch:
                    s, e = starts[k, b, g], ends[k, b, g]
                    cnt = e - s
                    o0 = run_off[b, g]
                    bucket_mms = [
                        (m, t)
                        for t in range(o0 // 128, (o0 + P[b, g] - 1) // 128 + 1)
                        for (m, gg) in block_mms[t]
                        if gg == g
                    ]
                    if cnt == 0:
                        continue
                    dvals = ((s_dst[s:e] % NPC) - g * GRP).astype(np.float32)
                    wvals = deg_out[s_src[s:e]]
                    for m, t in bucket_mms:
                        w0 = t * 128
                        lo = max(o0, w0)
                        hi = min(o0 + cnt, w0 + 128)
                        if lo >= hi:
                            continue
                        rows = np.arange(lo - w0, hi - w0)
                        doff[m, rows] = dvals[lo - o0 : hi - o0]
                        wdeg[m, rows] = wvals[lo - o0 : hi - o0]
        gidx_cores.append(_wrap16(gidx, nidx_tot // 16))
        dstoff_cores.append(np.ascontiguousarray(doff.T))
        wdeg_cores.append(np.ascontiguousarray(wdeg.T))

    # gather batches: split segments longer than BATCH_BLOCKS
    batches = []  # (bank, first_block, n_blocks)
    for b, seg0, nblk in segments:
        t = seg0 // 128
        left = nblk
        while left > 0:
            n = min(left, cfg["BATCH_BLOCKS"])
            batches.append((b, t, n))
            t += n
            left -= n

    meta = dict(
        NPC=NPC,
        NG=NG,
        BROWS=BROWS,
        nidx_tot=nidx_tot,
        nb_tot=nb_tot,
        n_mm=n_mm,
        block_mms=block_mms,
        mm_flags=mm_flags,
        batches=batches,
        deg_in=deg_in,
    )
    return meta, gidx_cores, dstoff_cores, wdeg_cores


def _tile_major(vec, NG, GRP, pad_val):
    """[NPC] -> [GRP, NG]: entry (p, m) = vec[m*GRP + p], padded."""
    out = np.full((NG * GRP,), pad_val, vec.dtype)
    out[: vec.shape[0]] = vec
    return np.ascontiguousarray(out.reshape(NG, GRP).T)


def _build_nc(cfg, meta):
    N, IN, OUT, C = cfg["N"], cfg["IN"], cfg["OUT"], cfg["NCORES"]
    GRP, NBANKS, GC = cfg["GRP"], cfg["NBANKS"], cfg["GCHUNK"]
    NPC, NG, BROWS = meta["NPC"], meta["NG"], meta["BROWS"]
    nidx_tot, nb_tot = meta["nidx_tot"], meta["nb_tot"]
    n_mm = meta["n_mm"]
    block_mms, mm_flags = meta["block_mms"], meta["mm_flags"]
    batches = meta["batches"]
    XK = _ceil_div(IN, 128)
    assert OUT == 128 and GRP == 128 and XK == 2
    last_w = NPC - (NG - 1) * GRP  # valid rows in the last group

    nc = bacc.Bacc(
        "TRN2", target_bir_lowering=False, debug=False, num_devices=C
    )

    # ---- external inputs ----
    xb = [
        nc.dram_tensor(f"xb{q}", [BROWS, IN], BF16, kind="ExternalInput")
        for q in range(NBANKS)
    ]
    wt = [
        nc.dram_tensor(f"wt{j}", [128, OUT], BF16, kind="ExternalInput")
        for j in range(XK)
    ]
    gidx_d = nc.dram_tensor("gidx", [128, nidx_tot // 16], I16, kind="ExternalInput")
    doff_d = nc.dram_tensor("doff", [128, n_mm], F32, kind="ExternalInput")
    wdeg_d = nc.dram_tensor("wdeg", [128, n_mm], F32, kind="ExternalInput")
    degi_d = nc.dram_tensor("degi", [128, NG], F32, kind="ExternalInput")
    bt_d = nc.dram_tensor("bt", [128, OUT], F32, kind="ExternalInput")
    iota_d = nc.dram_tensor("iota", [128, GRP], BF16, kind="ExternalInput")
    ident_d = nc.dram_tensor("ident", [128, 128], BF16, kind="ExternalInput")
    gm_d = nc.dram_tensor("gm", [1, OUT], F32, kind="ExternalInput")
    bb_d = nc.dram_tensor("bb", [1, OUT], F32, kind="ExternalInput")
    onesc_d = nc.dram_tensor("onesc", [128, 1], BF16, kind="ExternalInput")
    ones8_d = nc.dram_tensor("ones8", [8, 1], F32, kind="ExternalInput")
    onest_d = nc.dram_tensor("onest", [128, 1], BF16, kind="ExternalInput")
    onesr_d = nc.dram_tensor("onesr", [1, 128], F32, kind="ExternalInput")

    ypad_d = nc.dram_tensor("ypad", [NG * GRP, OUT], BF16, kind="ExternalOutput")

    with tile.TileContext(nc) as tc:
        with (
            tc.tile_pool(name="const", bufs=1) as cpool,
            tc.tile_pool(name="dram", bufs=1, space="DRAM") as dpool,
            tc.tile_pool(name="hrelu", bufs=1) as hpool,
            tc.tile_pool(name="gath", bufs=5) as gpool,
            tc.tile_pool(name="mpool", bufs=10) as mpool,
            tc.tile_pool(name="capool", bufs=6) as capool,
            tc.tile_pool(name="etmp", bufs=6) as epool,
            tc.tile_pool(name="gtmp", bufs=4) as gpool2,
            tc.tile_pool(name="psagg", bufs=1, space="PSUM") as pagg,
            tc.tile_pool(name="psaux", bufs=1, space="PSUM") as paux,
            tc.tile_pool(name="pstat", bufs=1, space="PSUM") as pspool,
        ):
            # ---- constants / small tiles ----
            iota_t = cpool.tile([128, GRP], BF16)
            ident_t = cpool.tile([128, 128], BF16)
            bt_t = cpool.tile([128, OUT], F32)
            degi_t = cpool.tile([128, NG], F32)
            ndst_t = cpool.tile([128, NG], F32)
            gm_t = cpool.tile([1, OUT], F32)
            bb_t = cpool.tile([1, OUT], F32)
            onesc_t = cpool.tile([128, 1], BF16)
            ones8_t = cpool.tile([8, 1], F32)
            onest_t = cpool.tile([128, 1], BF16)
            onesr_t = cpool.tile([1, 128], F32)
            gidx_t = cpool.tile([128, nidx_tot // 16], I16)
            doff_t = cpool.tile([128, n_mm], F32)
            wsrc_t = cpool.tile([128, n_mm], F32)
            wts = [cpool.tile([128, OUT], BF16, name=f"wt_s{j}") for j in range(XK)]

            # split the big index loads so the first gathers aren't gated on
            # the full-table DMA
            gsplit = min(nidx_tot // 16, 256)
            nc.sync.dma_start(gidx_t[:, :gsplit], gidx_d[:, :gsplit])
            if gsplit < nidx_tot // 16:
                nc.sync.dma_start(gidx_t[:, gsplit:], gidx_d[:, gsplit:])
            dsplit = min(n_mm, 128)
            nc.sync.dma_start(doff_t[:, :dsplit], doff_d[:, :dsplit])
            nc.sync.dma_start(wsrc_t[:, :dsplit], wdeg_d[:, :dsplit])
            if dsplit < n_mm:
                nc.sync.dma_start(doff_t[:, dsplit:], doff_d[:, dsplit:])
                nc.sync.dma_start(wsrc_t[:, dsplit:], wdeg_d[:, dsplit:])
            nc.sync.dma_start(iota_t[:], iota_d[:])
            nc.sync.dma_start(ident_t[:], ident_d[:])
            nc.sync.dma_start(bt_t[:], bt_d[:])
            nc.sync.dma_start(degi_t[:], degi_d[:])
            nc.sync.dma_start(gm_t[:], gm_d[:])
            nc.sync.dma_start(bb_t[:], bb_d[:])
            nc.sync.dma_start(onesc_t[:], onesc_d[:])
            nc.sync.dma_start(ones8_t[:], ones8_d[:])
            nc.sync.dma_start(onest_t[:], onest_d[:])
            nc.sync.dma_start(onesr_t[:], onesr_d[:])
            for j in range(XK):
                nc.sync.dma_start(wts[j][:], wt[j][:])

            # per-edge src norm: w = rsqrt(deg_out[src]) (pad slots carry 1.0);
            # two pieces so the first masks aren't gated on the full tile
            for c0, c1 in ((0, dsplit), (dsplit, n_mm)):
                if c0 >= c1:
                    continue
                nc.vector.reciprocal(wsrc_t[:, c0:c1], wsrc_t[:, c0:c1])
                nc.scalar.activation(
                    wsrc_t[:, c0:c1], wsrc_t[:, c0:c1],
                    mybir.ActivationFunctionType.Sqrt,
                )
            # dst norm: rsqrt(max(deg_in, 1)) tile-major [GRP, NG]
            nc.vector.tensor_scalar(
                ndst_t[:], degi_t[:], 1.0, None, op0=mybir.AluOpType.max
            )
            nc.vector.reciprocal(ndst_t[:], ndst_t[:])
            nc.scalar.activation(
                ndst_t[:], ndst_t[:], mybir.ActivationFunctionType.Sqrt
            )

            stats_in = dpool.tile([1, 2 * OUT], F32)
            _aspace = "Local" if cfg.get("NOCC") else "Shared"
            stats_out = dpool.tile([C, 2 * OUT], F32, addr_space=_aspace)

            hrelu_t = hpool.tile([128, NG, OUT], BF16)

            # ---- PSUM layout (8 banks x 2KB); accumulation-group zeroing is
            # bank-granular, so every concurrently-open chain gets its own
            # bank: 4x agg (GCHUNK groups in flight) + 1x transpose + 2x out
            # (alternating, WAR-tracked) + 1x BN stats (sum+sq as one chain).
            assert GC == 4
            ps_agg = [
                pagg.tile([128, 2 * OUT], F32, name=f"ps_agg{i}") for i in range(GC)
            ]
            ps_tr = paux.tile([128, 2, OUT], BF16, name="ps_tr")
            ps_out = [
                paux.tile([128, OUT], F32, name=f"ps_out{i}") for i in range(2)
            ]
            ps_stat = pspool.tile([1, 2, OUT], F32, name="ps_stat")

            # ---- stage D: gather x rows + one-hot matmul segmented sum ----
            def _finish_group(g):
                """Group g's PSUM agg is complete: apply W, relu, BN partials."""
                gi = g % GC
                cagg = capool.tile([128, 2 * OUT], BF16, tag="cagg")
                nc.scalar.activation(
                    cagg[:], ps_agg[gi][:], mybir.ActivationFunctionType.Copy
                )
                for h in range(2):
                    nc.tensor.matmul(
                        ps_tr[:, h, :],
                        cagg[:, h * OUT : (h + 1) * OUT],
                        ident_t[:],
                        is_transpose=True,
                        start=(h == 0),
                        stop=(h == 1),
                    )
                # one whole-tile copy so the next group's transpose chain
                # (which pending-zeroes the full bank) WAR-waits on it
                tagg = capool.tile([128, 2, OUT], BF16, tag="tagg")
                nc.scalar.activation(
                    tagg[:, :, :], ps_tr[:, :, :],
                    mybir.ActivationFunctionType.Copy,
                )
                po = ps_out[g % 2]
                for jj in range(XK):
                    nc.tensor.matmul(
                        po[:],
                        tagg[:, jj, :],
                        wts[jj][:],
                        start=(jj == 0),
                        stop=(jj == XK - 1),
                    )
                # stage E: relu(out * ndst + b), BN partial sums
                tmp = epool.tile([128, OUT], F32, tag="etmp")
                nc.vector.scalar_tensor_tensor(
                    tmp[:],
                    po[:],
                    ndst_t[:, g : g + 1],
                    bt_t[:],
                    op0=mybir.AluOpType.mult,
                    op1=mybir.AluOpType.add,
                )
                nc.scalar.activation(
                    hrelu_t[:, g, :], tmp[:], mybir.ActivationFunctionType.Relu
                )
                ones = onesc_t if g < NG - 1 else onest_t
                nc.tensor.matmul(
                    ps_stat[:, 0, :],
                    ones[:],
                    hrelu_t[:, g, :],
                    start=(g == 0),
                    stop=False,
                )
                sq = epool.tile([128, OUT], BF16, tag="esq")
                nc.scalar.activation(
                    sq[:], hrelu_t[:, g, :], mybir.ActivationFunctionType.Square
                )
                nc.tensor.matmul(
                    ps_stat[:, 1, :],
                    ones[:],
                    sq[:],
                    start=False,
                    stop=(g == NG - 1),
                )

            bmax = max(nb for _, _, nb in batches)
            for bank, t0, nblk in batches:
                Gt = gpool.tile([128, bmax, IN], BF16, tag="G")
                nc.gpsimd.dma_gather(
                    Gt[:, :nblk, :],
                    xb[bank][:],
                    gidx_t[:, t0 * 8 : (t0 + nblk) * 8],
                    nblk * 128,
                    nblk * 128,
                    IN,
                    single_packet=False,
                )
                for j in range(nblk):
                    t = t0 + j
                    for m, g in block_mms[t]:
                      is_start, is_stop = mm_flags[m]
                      gi = g % GC
                      Mt = mpool.tile([128, GRP], BF16, tag="M")
                      nc.vector.tensor_scalar(
                          Mt[:],
                          iota_t[:],
                          doff_t[:, m : m + 1],
                          wsrc_t[:, m : m + 1],
                          op0=mybir.AluOpType.is_equal,
                          op1=mybir.AluOpType.mult,
                      )
                      nc.tensor.matmul(
                          ps_agg[gi][:],
                          Mt[:],
                          Gt[:, j, :],
                          start=is_start,
                          stop=is_stop,
                      )
                      if is_stop:
                          _finish_group(g)

            # ---- stage F: AllReduce BN stats; build affine S/T tiles ----
            S_t = cpool.tile([128, OUT], BF16)
            T_t = cpool.tile([128, OUT], BF16)
            st_sb = cpool.tile([1, 2 * OUT], F32)
            nc.scalar.activation(
                st_sb[:], ps_stat[:].rearrange("p a f -> p (a f)"),
                mybir.ActivationFunctionType.Copy,
            )
            nc.sync.dma_start(stats_in[:], st_sb[:])
            if cfg.get("NOCC"):
                stats_out = stats_in  # single-core debug: sums are the totals
                st8 = st_sb
                ones8v = None
            else:
                # AllGather (no 1.875x reduce multiplier) + tiny local
                # ones-matmul reduction beats AllReduce on latency
                nc.gpsimd.collective_compute(
                    "AllGather",
                    mybir.AluOpType.bypass,
                    replica_groups=[list(range(C))],
                    ins=[stats_in[:]],
                    outs=[stats_out[:]],
                )
                st8 = cpool.tile([C, 2 * OUT], F32)
                nc.sync.dma_start(st8[:], stats_out[:])
                ones8v = ones8_t
            st_rb = cpool.tile([1, 2 * OUT], F32)
            if ones8v is None:
                nc.scalar.activation(
                    st_rb[:], st8[:], mybir.ActivationFunctionType.Copy
                )
            else:
                ps_red = ps_stat[:].rearrange("p a f -> p (a f)")
                nc.tensor.matmul(ps_red, ones8v[:], st8[:], start=True, stop=True)
                nc.scalar.activation(
                    st_rb[:], ps_red, mybir.ActivationFunctionType.Copy
                )

            mu = cpool.tile([1, OUT], F32)
            musq = cpool.tile([1, OUT], F32)
            var = cpool.tile([1, OUT], F32)
            srow = cpool.tile([1, OUT], F32)
            trow = cpool.tile([1, OUT], F32)
            inv_n = 1.0 / float(N)
            nc.scalar.activation(
                mu[:], st_rb[:, 0:OUT], mybir.ActivationFunctionType.Copy, scale=inv_n
            )
            nc.scalar.activation(
                musq[:], mu[:], mybir.ActivationFunctionType.Square
            )
            # var + eps = (E[x^2]*inv_n + eps) - mu^2, then rsqrt via
            # reciprocal+sqrt (ACT Rsqrt is banned for accuracy)
            nc.scalar.activation(
                st_rb[:, OUT : 2 * OUT],
                st_rb[:, OUT : 2 * OUT],
                mybir.ActivationFunctionType.Copy,
                scale=inv_n,
                bias=float(cfg["EPS"]),
            )
            nc.vector.tensor_sub(var[:], st_rb[:, OUT : 2 * OUT], musq[:])
            nc.vector.reciprocal(var[:], var[:])
            nc.scalar.activation(
                var[:], var[:], mybir.ActivationFunctionType.Sqrt
            )
            nc.vector.tensor_mul(srow[:], gm_t[:], var[:])
            nc.vector.tensor_mul(trow[:], mu[:], srow[:])
            nc.vector.tensor_sub(trow[:], bb_t[:], trow[:])

            # reuse the (now idle) out banks for the S/T broadcast matmuls
            nc.tensor.matmul(ps_out[0][:], onesr_t[:], srow[:], start=True, stop=True)
            nc.scalar.activation(
                S_t[:], ps_out[0][:], mybir.ActivationFunctionType.Copy
            )
            nc.tensor.matmul(ps_out[1][:], onesr_t[:], trow[:], start=True, stop=True)
            nc.scalar.activation(
                T_t[:], ps_out[1][:], mybir.ActivationFunctionType.Copy
            )

            # ---- stage G: y = hrelu * S + T (in place, S/T broadcast along
            # the group axis), output DMA chunked to overlap with the DVE ----
            ypad_view = ypad_d[:].rearrange("(g p) f -> p g f", p=128)
            GOUT = 13
            for ci, c0 in enumerate(range(0, NG, GOUT)):
                c1 = min(c0 + GOUT, NG)
                S_b = S_t[:].rearrange("p (a f) -> p a f", a=1).to_broadcast(
                    (128, c1 - c0, OUT)
                )
                T_b = T_t[:].rearrange("p (a f) -> p a f", a=1).to_broadcast(
                    (128, c1 - c0, OUT)
                )
                eng = nc.gpsimd if ci % 3 == 1 else nc.vector
                eng.tensor_mul(
                    hrelu_t[:, c0:c1, :], hrelu_t[:, c0:c1, :], S_b
                )
                eng.tensor_add(
                    hrelu_t[:, c0:c1, :], hrelu_t[:, c0:c1, :], T_b
                )
                nc.sync.dma_start(
                    ypad_view[:, c0:c1, :], hrelu_t[:, c0:c1, :]
                )

    nc.compile()
    return nc


def kernel(x, src, dst, W, b, gamma, beta):
    global LAST_RESULTS
    cfg = CFG
    N, E, IN, OUT, C = cfg["N"], cfg["E"], cfg["IN"], cfg["OUT"], cfg["NCORES"]
    GRP = cfg["GRP"]
    assert x.shape == (N, IN) and W.shape == (IN, OUT)
    assert src.shape == (E,) and dst.shape == (E,)

    meta, gidx_cores, dstoff_cores, wdeg_cores = _preprocess(cfg, src, dst)
    NPC, NG, BROWS = meta["NPC"], meta["NG"], meta["BROWS"]
    XK = _ceil_div(IN, 128)
    last_w = NPC - (NG - 1) * GRP

    nc = _build_nc(cfg, meta)

    import ml_dtypes

    x_bf = np.asarray(x, np.float32).astype(ml_dtypes.bfloat16)
    Wn = np.asarray(W, np.float32)

    iota = np.tile(
        np.arange(GRP, dtype=np.float32)[None, :], (128, 1)
    ).astype(ml_dtypes.bfloat16)
    ident = np.eye(128, dtype=np.float32).astype(ml_dtypes.bfloat16)
    bt = np.tile(np.asarray(b, np.float32)[None, :], (128, 1))
    onesc = np.ones((128, 1), np.float32).astype(ml_dtypes.bfloat16)
    ones8 = np.ones((8, 1), np.float32)
    onest = np.zeros((128, 1), np.float32)
    onest[:last_w] = 1.0
    onest = onest.astype(ml_dtypes.bfloat16)
    onesr = np.ones((1, 128), np.float32)
    gm = np.asarray(gamma, np.float32)[None, :]
    bb = np.asarray(beta, np.float32)[None, :]
    xbanks = [
        np.ascontiguousarray(x_bf[q * BROWS : (q + 1) * BROWS])
        for q in range(cfg["NBANKS"])
    ]
    wtiles = [
        np.ascontiguousarray(Wn[j * 128 : (j + 1) * 128, :]).astype(
            ml_dtypes.bfloat16
        )
        for j in range(XK)
    ]

    in_maps = []
    for k in range(C):
        im = {
            "gidx": gidx_cores[k],
            "doff": dstoff_cores[k],
            "wdeg": wdeg_cores[k],
            "degi": _tile_major(
                meta["deg_in"][k * NPC : (k + 1) * NPC], NG, GRP, np.float32(1.0)
            ),
            "bt": bt,
            "iota": iota,
            "ident": ident,
            "gm": gm,
            "bb": bb,
            "onesc": onesc,
            "ones8": ones8,
            "onest": onest,
            "onesr": onesr,
        }
        for q in range(cfg["NBANKS"]):
            im[f"xb{q}"] = xbanks[q]
        for j in range(XK):
            im[f"wt{j}"] = wtiles[j]
        in_maps.append(im)

    if cfg.get("SIM"):
        from concourse.bass_interp import MultiCoreSim

        sim = MultiCoreSim(nc, num_cores=C)
        for k, core_sim in sim.cores.items():
            for name, val in in_maps[k].items():
                core_sim.tensor(name)[:] = val
        sim.simulate()
        y = np.empty((N, OUT), np.float32)
        for k in range(C):
            y[k * NPC : (k + 1) * NPC] = np.asarray(sim.cores[k].tensor("ypad")[:NPC], dtype=np.float32)
        return y

    global LAST_NC, LAST_RUN_S
    LAST_NC = nc
    import time as _time

    _t0 = _time.time()
    res = bass_utils.run_bass_kernel_spmd(
        nc,
        in_maps,
        core_ids=list(range(C)),
        trace=cfg.get("TRACE", False),
    )
    LAST_RUN_S = _time.time() - _t0
    LAST_RESULTS = res

    y = np.empty((N, OUT), np.float32)
    for k in range(C):
        y[k * NPC : (k + 1) * NPC] = np.asarray(res.results[k]["ypad"][:NPC], dtype=np.float32)
    return y


# revision 6
# speedup vs baseline: 1.2950x; 1.0127x over previous
"""GCN block (GraphConv + BatchNorm1d + ReLU) on 8 Trainium2 NeuronCores.

v2 strategy — "gather x, apply W after aggregation":

By linearity, agg[dst] = sum_e norm_src[src_e] * x[src_e] @ W
                       = (sum_e norm_src[src_e] * x[src_e]) @ W.
So instead of computing h = x@W on every shard and AllGather-ing the h table
(collectives dominated the v1 timeline),每 core receives the FULL x (bf16,
row-major) in its own HBM and directly dma_gathers raw x rows for its edges.
No AllGather at all, and gathers start at t=0. x rows are 256 bf16 = 512 B,
which also clears the <512 B small-descriptor DMA penalty that h rows
(128 bf16 = 256 B) pay.

Per core k (owns dst nodes [k*NPC, (k+1)*NPC)):
  1. For each 128-edge block (bucketed by (dst-group, src-bank)), gather
     x[src] rows (bf16, batched dma_gather) and segment-sum with one-hot
     matmuls Mw^T @ G accumulated in PSUM. The one-hot mask is scaled by
     w_e = rsqrt(deg_out[src_e]) (tensor_scalar is_equal*mult), folding
     norm_src into the aggregation.
  2. Per finished dst group: agg_x [128,256] PSUM -> bf16 SBUF -> PE
     transpose -> (aggT_j)^T @ W_j accumulated -> out [128, OUT] PSUM.
  3. relu(out * rsqrt(clip(deg_in,1)) + b); local BN sums via ones-matmul;
     AllReduce BN sums; y = (h - mu) * rsqrt(var+eps) * gamma + beta.

Host-side work is limited to integer index bookkeeping (bucketing edges by
(core, src-bank, dst-group), degree counting) and layout transforms (bf16
cast, int16 gather indices). All floating-point math runs on device.

Edges are bucketed by src bank (4 banks of N/4 rows) because dma_gather
indices are int16 (< 32768). Bucket sizes are padded to a structure shared
by all 8 cores so a single SPMD NEFF serves every core; pad slots gather row
0 of the bank and carry a dst offset of 255 -> their one-hot column is all
zeros, so they contribute exactly 0. Banks 0 and 3 are padded to >= 1 block
per group so every group starts in bank 0 and stops in bank 3 (keeps the
BN-stat accumulation chain's start first / stop last in program order).
"""
import math
import os
import sys

sys.path.insert(0, "/opt/trn_rl_repo")

import numpy as np

import concourse.bacc as bacc
import concourse.bass as bass
import concourse.mybir as mybir
import concourse.tile as tile
from concourse import bass_utils

F32 = mybir.dt.float32
BF16 = mybir.dt.bfloat16
I16 = mybir.dt.int16

CFG = dict(
    N=100000,
    E=1600000,
    IN=256,
    OUT=128,
    NCORES=8,
    GRP=128,          # dst nodes per segment group (= psum partition dim)
    NBANKS=4,         # src banks (bank rows must stay < 32768 for int16 idx)
    GCHUNK=4,         # dst groups in flight (1 PSUM bank per open accum chain)
    BATCH_BLOCKS=40,  # gather batch size in 128-edge blocks
    EPS=1e-5,
    TRACE=False,
)

LAST_RESULTS = None  # set by kernel() for test harness introspection
LAST_NC = None
LAST_RUN_S = None


def _ceil_div(a, b):
    return (a + b - 1) // b


def _wrap16(idx, ncols):
    """int16 idx list -> [128, ncols] tile: idx i at [i%16, i//16], replicated
    8x across the 16-partition groups (one copy per GpSimd Q7 core)."""
    n = idx.shape[0]
    assert n == ncols * 16
    w = np.ascontiguousarray(idx.reshape(ncols, 16).T)
    return np.tile(w, (8, 1))


def _preprocess(cfg, src, dst):
    """Bucket edges by (owner core, src bank, dst group); build per-core
    gather-index / dst-offset / src-degree arrays and the shared block
    structure."""
    N, E = cfg["N"], cfg["E"]
    C, NBANKS, GRP = cfg["NCORES"], cfg["NBANKS"], cfg["GRP"]
    NPC = N // C
    NG = _ceil_div(NPC, GRP)
    assert N % NBANKS == 0
    BROWS = N // NBANKS            # rows per x bank (gather source table)
    assert BROWS < 32768

    src = src.astype(np.int64)
    dst = dst.astype(np.int64)
    deg_out = np.bincount(src, minlength=N).astype(np.float32)
    deg_in = np.bincount(dst, minlength=N).astype(np.float32)

    owner = dst // NPC
    bank = src // BROWS
    grp = (dst % NPC) // GRP
    key = (owner * NBANKS + bank) * NG + grp
    order = np.argsort(key, kind="stable")
    s_src = src[order]
    s_dst = dst[order]
    s_key = key[order]

    counts = np.bincount(key, minlength=C * NBANKS * NG).reshape(C, NBANKS, NG)
    # bucket capacity: exact max over cores (shared SPMD structure); >= 1 in
    # banks 0/3 so every group has a first (bank-0) and last (bank-3) matmul
    P = counts.max(axis=0)  # [NBANKS, NG]
    P[0] = np.maximum(P[0], 1)
    P[NBANKS - 1] = np.maximum(P[NBANKS - 1], 1)

    # stream order: group-chunks outer, banks inner; a group's PSUM slot is
    # live across all banks of its chunk (accumulated with start/stop).
    # Segment (chunk, bank) = that chunk's buckets concatenated, padded to a
    # multiple of 128; 128-edge blocks may straddle bucket (group) boundaries
    # -> one matmul per (block, overlapped group).
    GC = cfg["GCHUNK"]
    chunks = [list(range(c, min(c + GC, NG))) for c in range(0, NG, GC)]
    run_off = np.zeros((NBANKS, NG), np.int64)
    segments = []  # (bank, seg_start_slot, seg_nblocks)
    pos = 0
    for ch in chunks:
        for b in range(NBANKS):
            seg0 = pos
            for g in ch:
                run_off[b, g] = pos
                pos += P[b, g]
            pos = ((pos + 127) // 128) * 128  # segment tail pad
            segments.append((b, seg0, (pos - seg0) // 128))
    nidx_tot = pos
    nb_tot = nidx_tot // 128

    # per block: list of matmuls (mm_col, group); per group: first/last mm id
    block_mms = [[] for _ in range(nb_tot)]
    n_mm = 0
    mm_of_group = {}
    for ch in chunks:
        for b in range(NBANKS):
            for g in ch:
                o0, o1 = run_off[b, g], run_off[b, g] + P[b, g]
                for t in range(o0 // 128, (o1 - 1) // 128 + 1):
                    block_mms[t].append((n_mm, g))
                    mm_of_group.setdefault(g, []).append(n_mm)
                    n_mm += 1
    mm_flags = {}
    for g, mms in mm_of_group.items():
        for m in mms:
            mm_flags[m] = (m == mms[0], m == mms[-1])

    # boundaries of each (k, b, g) bucket in the sorted edge stream
    bkeys = (np.arange(C)[:, None, None] * NBANKS + np.arange(NBANKS)[None, :, None]) * NG + np.arange(NG)[None, None, :]
    starts = np.searchsorted(s_key, bkeys.ravel()).reshape(C, NBANKS, NG)
    ends = np.searchsorted(s_key, bkeys.ravel(), side="right").reshape(C, NBANKS, NG)

    gidx_cores = []
    dstoff_cores = []
    wdeg_cores = []
    for k in range(C):
        gidx = np.zeros(nidx_tot, np.int16)
        doff = np.full((n_mm, 128), 255.0, np.float32)
        wdeg = np.ones((n_mm, 128), np.float32)
        # fill gather indices per bucket (pad slots keep row 0)
        for b in range(NBANKS):
            for g in range(NG):
                s, e = starts[k, b, g], ends[k, b, g]
                cnt = e - s
                if cnt == 0:
                    continue
                p0 = run_off[b, g]
                gidx[p0 : p0 + cnt] = (s_src[s:e] % BROWS).astype(np.int16)
        # fill per-matmul mask columns: rows = this core's real edges of the
        # matmul's group that fall inside the block's 128-slot window
        for ch in chunks:
            for b in range(NBANKS):
                for g in ch:
                    s, e = starts[k, b, g], ends[k, b, g]
                    cnt = e - s
                    o0 = run_off[b, g]
                    bucket_mms = [
                        (m, t)
                        for t in range(o0 // 128, (o0 + P[b, g] - 1) // 128 + 1)
                        for (m, gg) in block_mms[t]
                        if gg == g
                    ]
                    if cnt == 0:
                        continue
                    dvals = ((s_dst[s:e] % NPC) - g * GRP).astype(np.float32)
                    wvals = deg_out[s_src[s:e]]
                    for m, t in bucket_mms:
                        w0 = t * 128
                        lo = max(o0, w0)
                        hi = min(o0 + cnt, w0 + 128)
                        if lo >= hi:
                            continue
                        rows = np.arange(lo - w0, hi - w0)
                        doff[m, rows] = dvals[lo - o0 : hi - o0]
                        wdeg[m, rows] = wvals[lo - o0 : hi - o0]
        gidx_cores.append(_wrap16(gidx, nidx_tot // 16))
        dstoff_cores.append(np.ascontiguousarray(doff.T))
        wdeg_cores.append(np.ascontiguousarray(wdeg.T))

    # gather batches: split segments longer than BATCH_BLOCKS
    batches = []  # (bank, first_block, n_blocks)
    for b, seg0, nblk in segments:
        t = seg0 // 128
        left = nblk
        while left > 0:
            n = min(left, cfg["BATCH_BLOCKS"])
            batches.append((b, t, n))
            t += n
            left -= n

    meta = dict(
        NPC=NPC,
        NG=NG,
        BROWS=BROWS,
        nidx_tot=nidx_tot,
        nb_tot=nb_tot,
        n_mm=n_mm,
        block_mms=block_mms,
        mm_flags=mm_flags,
        batches=batches,
        deg_in=deg_in,
    )
    return meta, gidx_cores, dstoff_cores, wdeg_cores


def _tile_major(vec, NG, GRP, pad_val):
    """[NPC] -> [GRP, NG]: entry (p, m) = vec[m*GRP + p], padded."""
    out = np.full((NG * GRP,), pad_val, vec.dtype)
    out[: vec.shape[0]] = vec
    return np.ascontiguousarray(out.reshape(NG, GRP).T)


def _build_nc(cfg, meta):
    N, IN, OUT, C = cfg["N"], cfg["IN"], cfg["OUT"], cfg["NCORES"]
    GRP, NBANKS, GC = cfg["GRP"], cfg["NBANKS"], cfg["GCHUNK"]
    NPC, NG, BROWS = meta["NPC"], meta["NG"], meta["BROWS"]
    nidx_tot, nb_tot = meta["nidx_tot"], meta["nb_tot"]
    n_mm = meta["n_mm"]
    block_mms, mm_flags = meta["block_mms"], meta["mm_flags"]
    batches = meta["batches"]
    XK = _ceil_div(IN, 128)
    assert OUT == 128 and GRP == 128 and XK == 2
    last_w = NPC - (NG - 1) * GRP  # valid rows in the last group

    nc = bacc.Bacc(
        "TRN2", target_bir_lowering=False, debug=False, num_devices=C
    )

    # ---- external inputs ----
    xb = [
        nc.dram_tensor(f"xb{q}", [BROWS, IN], BF16, kind="ExternalInput")
        for q in range(NBANKS)
    ]
    wt = [
        nc.dram_tensor(f"wt{j}", [128, OUT], BF16, kind="ExternalInput")
        for j in range(XK)
    ]
    gidx_d = nc.dram_tensor("gidx", [128, nidx_tot // 16], I16, kind="ExternalInput")
    doff_d = nc.dram_tensor("doff", [128, n_mm], F32, kind="ExternalInput")
    wdeg_d = nc.dram_tensor("wdeg", [128, n_mm], F32, kind="ExternalInput")
    degi_d = nc.dram_tensor("degi", [128, NG], F32, kind="ExternalInput")
    bt_d = nc.dram_tensor("bt", [128, OUT], F32, kind="ExternalInput")
    iota_d = nc.dram_tensor("iota", [128, GRP], BF16, kind="ExternalInput")
    ident_d = nc.dram_tensor("ident", [128, 128], BF16, kind="ExternalInput")
    gm_d = nc.dram_tensor("gm", [1, OUT], F32, kind="ExternalInput")
    bb_d = nc.dram_tensor("bb", [1, OUT], F32, kind="ExternalInput")
    onesc_d = nc.dram_tensor("onesc", [128, 1], BF16, kind="ExternalInput")
    ones8_d = nc.dram_tensor("ones8", [8, 1], F32, kind="ExternalInput")
    onest_d = nc.dram_tensor("onest", [128, 1], BF16, kind="ExternalInput")
    onesr_d = nc.dram_tensor("onesr", [1, 128], F32, kind="ExternalInput")

    ypad_d = nc.dram_tensor("ypad", [NG * GRP, OUT], BF16, kind="ExternalOutput")

    with tile.TileContext(nc) as tc:
        with (
            tc.tile_pool(name="const", bufs=1) as cpool,
            tc.tile_pool(name="dram", bufs=1, space="DRAM") as dpool,
            tc.tile_pool(name="hrelu", bufs=1) as hpool,
            tc.tile_pool(name="gath", bufs=5) as gpool,
            tc.tile_pool(name="mpool", bufs=10) as mpool,
            tc.tile_pool(name="capool", bufs=6) as capool,
            tc.tile_pool(name="etmp", bufs=6) as epool,
            tc.tile_pool(name="gtmp", bufs=4) as gpool2,
            tc.tile_pool(name="psagg", bufs=1, space="PSUM") as pagg,
            tc.tile_pool(name="psaux", bufs=1, space="PSUM") as paux,
            tc.tile_pool(name="pstat", bufs=1, space="PSUM") as pspool,
        ):
            # ---- constants / small tiles ----
            iota_t = cpool.tile([128, GRP], BF16)
            ident_t = cpool.tile([128, 128], BF16)
            bt_t = cpool.tile([128, OUT], F32)
            degi_t = cpool.tile([128, NG], F32)
            ndst_t = cpool.tile([128, NG], F32)
            gm_t = cpool.tile([1, OUT], F32)
            bb_t = cpool.tile([1, OUT], F32)
            onesc_t = cpool.tile([128, 1], BF16)
            ones8_t = cpool.tile([8, 1], F32)
            onest_t = cpool.tile([128, 1], BF16)
            onesr_t = cpool.tile([1, 128], F32)
            gidx_t = cpool.tile([128, nidx_tot // 16], I16)
            doff_t = cpool.tile([128, n_mm], F32)
            wsrc_t = cpool.tile([128, n_mm], F32)
            wts = [cpool.tile([128, OUT], BF16, name=f"wt_s{j}") for j in range(XK)]

            # split the big index loads so the first gathers aren't gated on
            # the full-table DMA
            gsplit = min(nidx_tot // 16, 256)
            nc.sync.dma_start(gidx_t[:, :gsplit], gidx_d[:, :gsplit])
            if gsplit < nidx_tot // 16:
                nc.sync.dma_start(gidx_t[:, gsplit:], gidx_d[:, gsplit:])
            dsplit = min(n_mm, 128)
            nc.sync.dma_start(doff_t[:, :dsplit], doff_d[:, :dsplit])
            nc.sync.dma_start(wsrc_t[:, :dsplit], wdeg_d[:, :dsplit])
            if dsplit < n_mm:
                nc.sync.dma_start(doff_t[:, dsplit:], doff_d[:, dsplit:])
                nc.sync.dma_start(wsrc_t[:, dsplit:], wdeg_d[:, dsplit:])
            nc.sync.dma_start(iota_t[:], iota_d[:])
            nc.sync.dma_start(ident_t[:], ident_d[:])
            nc.sync.dma_start(bt_t[:], bt_d[:])
            nc.sync.dma_start(degi_t[:], degi_d[:])
            nc.sync.dma_start(gm_t[:], gm_d[:])
            nc.sync.dma_start(bb_t[:], bb_d[:])
            nc.sync.dma_start(onesc_t[:], onesc_d[:])
            nc.sync.dma_start(ones8_t[:], ones8_d[:])
            nc.sync.dma_start(onest_t[:], onest_d[:])
            nc.sync.dma_start(onesr_t[:], onesr_d[:])
            for j in range(XK):
                nc.sync.dma_start(wts[j][:], wt[j][:])

            # per-edge src norm: w = rsqrt(deg_out[src]) (pad slots carry 1.0);
            # two pieces so the first masks aren't gated on the full tile
            for c0, c1 in ((0, dsplit), (dsplit, n_mm)):
                if c0 >= c1:
                    continue
                nc.vector.reciprocal(wsrc_t[:, c0:c1], wsrc_t[:, c0:c1])
                nc.scalar.activation(
                    wsrc_t[:, c0:c1], wsrc_t[:, c0:c1],
                    mybir.ActivationFunctionType.Sqrt,
                )
            # dst norm: rsqrt(max(deg_in, 1)) tile-major [GRP, NG]
            nc.vector.tensor_scalar(
                ndst_t[:], degi_t[:], 1.0, None, op0=mybir.AluOpType.max
            )
            nc.vector.reciprocal(ndst_t[:], ndst_t[:])
            nc.scalar.activation(
                ndst_t[:], ndst_t[:], mybir.ActivationFunctionType.Sqrt
            )

            stats_in = dpool.tile([1, 2 * OUT], F32)
            _aspace = "Local" if cfg.get("NOCC") else "Shared"
            stats_out = dpool.tile([C, 2 * OUT], F32, addr_space=_aspace)

            hrelu_t = hpool.tile([128, NG, OUT], BF16)

            # ---- PSUM layout (8 banks x 2KB); accumulation-group zeroing is
            # bank-granular, so every concurrently-open chain gets its own
            # bank: 4x agg (GCHUNK groups in flight) + 1x transpose + 2x out
            # (alternating, WAR-tracked) + 1x BN stats (sum+sq as one chain).
            assert GC == 4
            ps_agg = [
                pagg.tile([128, 2 * OUT], F32, name=f"ps_agg{i}") for i in range(GC)
            ]
            ps_tr = paux.tile([128, 2, OUT], BF16, name="ps_tr")
            ps_out = [
                paux.tile([128, OUT], F32, name=f"ps_out{i}") for i in range(2)
            ]
            ps_stat = pspool.tile([1, 2, OUT], F32, name="ps_stat")

            # ---- stage D: gather x rows + one-hot matmul segmented sum ----
            def _finish_group(g):
                """Group g's PSUM agg is complete: apply W, relu, BN partials."""
                gi = g % GC
                cagg = capool.tile([128, 2 * OUT], BF16, tag="cagg")
                nc.scalar.activation(
                    cagg[:], ps_agg[gi][:], mybir.ActivationFunctionType.Copy
                )
                for h in range(2):
                    nc.tensor.matmul(
                        ps_tr[:, h, :],
                        cagg[:, h * OUT : (h + 1) * OUT],
                        ident_t[:],
                        is_transpose=True,
                        start=(h == 0),
                        stop=(h == 1),
                    )
                # one whole-tile copy so the next group's transpose chain
                # (which pending-zeroes the full bank) WAR-waits on it
                tagg = capool.tile([128, 2, OUT], BF16, tag="tagg")
                nc.scalar.activation(
                    tagg[:, :, :], ps_tr[:, :, :],
                    mybir.ActivationFunctionType.Copy,
                )
                po = ps_out[g % 2]
                for jj in range(XK):
                    nc.tensor.matmul(
                        po[:],
                        tagg[:, jj, :],
                        wts[jj][:],
                        start=(jj == 0),
                        stop=(jj == XK - 1),
                    )
                # stage E: relu(out * ndst + b), BN partial sums
                tmp = epool.tile([128, OUT], F32, tag="etmp")
                nc.vector.scalar_tensor_tensor(
                    tmp[:],
                    po[:],
                    ndst_t[:, g : g + 1],
                    bt_t[:],
                    op0=mybir.AluOpType.mult,
                    op1=mybir.AluOpType.add,
                )
                nc.scalar.activation(
                    hrelu_t[:, g, :], tmp[:], mybir.ActivationFunctionType.Relu
                )
                ones = onesc_t if g < NG - 1 else onest_t
                nc.tensor.matmul(
                    ps_stat[:, 0, :],
                    ones[:],
                    hrelu_t[:, g, :],
                    start=(g == 0),
                    stop=False,
                )
                sq = epool.tile([128, OUT], BF16, tag="esq")
                nc.scalar.activation(
                    sq[:], hrelu_t[:, g, :], mybir.ActivationFunctionType.Square
                )
                nc.tensor.matmul(
                    ps_stat[:, 1, :],
                    ones[:],
                    sq[:],
                    start=False,
                    stop=(g == NG - 1),
                )

            bmax = max(nb for _, _, nb in batches)
            for bank, t0, nblk in batches:
                Gt = gpool.tile([128, bmax, IN], BF16, tag="G")
                nc.gpsimd.dma_gather(
                    Gt[:, :nblk, :],
                    xb[bank][:],
                    gidx_t[:, t0 * 8 : (t0 + nblk) * 8],
                    nblk * 128,
                    nblk * 128,
                    IN,
                    single_packet=False,
                )
                for j in range(nblk):
                    t = t0 + j
                    for m, g in block_mms[t]:
                      is_start, is_stop = mm_flags[m]
                      gi = g % GC
                      Mt = mpool.tile([128, GRP], BF16, tag="M")
                      nc.vector.tensor_scalar(
                          Mt[:],
                          iota_t[:],
                          doff_t[:, m : m + 1],
                          wsrc_t[:, m : m + 1],
                          op0=mybir.AluOpType.is_equal,
                          op1=mybir.AluOpType.mult,
                      )
                      nc.tensor.matmul(
                          ps_agg[gi][:],
                          Mt[:],
                          Gt[:, j, :],
                          start=is_start,
                          stop=is_stop,
                      )
                      if is_stop:
                          _finish_group(g)

            # ---- stage F: AllReduce BN stats; build affine S/T tiles ----
            S_t = cpool.tile([128, OUT], BF16)
            T_t = cpool.tile([128, OUT], BF16)
            st_sb = cpool.tile([1, 2 * OUT], F32)
            nc.scalar.activation(
                st_sb[:], ps_stat[:].rearrange("p a f -> p (a f)"),
                mybir.ActivationFunctionType.Copy,
            )
            nc.sync.dma_start(stats_in[:], st_sb[:])
            if cfg.get("NOCC"):
                stats_out = stats_in  # single-core debug: sums are the totals
                st8 = st_sb
                ones8v = None
            else:
                # AllGather (no 1.875x reduce multiplier) + tiny local
                # ones-matmul reduction beats AllReduce on latency
                nc.gpsimd.collective_compute(
                    "AllGather",
                    mybir.AluOpType.bypass,
                    replica_groups=[list(range(C))],
                    ins=[stats_in[:]],
                    outs=[stats_out[:]],
                )
                st8 = cpool.tile([C, 2 * OUT], F32)
                nc.sync.dma_start(st8[:], stats_out[:])
                ones8v = ones8_t
            st_rb = cpool.tile([1, 2 * OUT], F32)
            if ones8v is None:
                nc.scalar.activation(
                    st_rb[:], st8[:], mybir.ActivationFunctionType.Copy
                )
            else:
                ps_red = ps_stat[:].rearrange("p a f -> p (a f)")
                nc.tensor.matmul(ps_red, ones8v[:], st8[:], start=True, stop=True)
                nc.scalar.activation(
                    st_rb[:], ps_red, mybir.ActivationFunctionType.Copy
                )

            mu = cpool.tile([1, OUT], F32)
            musq = cpool.tile([1, OUT], F32)
            var = cpool.tile([1, OUT], F32)
            srow = cpool.tile([1, OUT], F32)
            trow = cpool.tile([1, OUT], F32)
            inv_n = 1.0 / float(N)
            nc.scalar.activation(
                mu[:], st_rb[:, 0:OUT], mybir.ActivationFunctionType.Copy, scale=inv_n
            )
            nc.scalar.activation(
                musq[:], mu[:], mybir.ActivationFunctionType.Square
            )
            # var + eps = (E[x^2]*inv_n + eps) - mu^2, then rsqrt via
            # reciprocal+sqrt (ACT Rsqrt is banned for accuracy)
            nc.scalar.activation(
                st_rb[:, OUT : 2 * OUT],
                st_rb[:, OUT : 2 * OUT],
                mybir.ActivationFunctionType.Copy,
                scale=inv_n,
                bias=float(cfg["EPS"]),
            )
            nc.vector.tensor_sub(var[:], st_rb[:, OUT : 2 * OUT], musq[:])
            nc.vector.reciprocal(var[:], var[:])
            nc.scalar.activation(
                var[:], var[:], mybir.ActivationFunctionType.Sqrt
            )
            nc.vector.tensor_mul(srow[:], gm_t[:], var[:])
            nc.vector.tensor_mul(trow[:], mu[:], srow[:])
            nc.vector.tensor_sub(trow[:], bb_t[:], trow[:])

            # reuse the (now idle) out banks for the S/T broadcast matmuls
            nc.tensor.matmul(ps_out[0][:], onesr_t[:], srow[:], start=True, stop=True)
            nc.scalar.activation(
                S_t[:], ps_out[0][:], mybir.ActivationFunctionType.Copy
            )
            nc.tensor.matmul(ps_out[1][:], onesr_t[:], trow[:], start=True, stop=True)
            nc.scalar.activation(
                T_t[:], ps_out[1][:], mybir.ActivationFunctionType.Copy
            )

            # ---- stage G: y = hrelu * S + T (in place, S/T broadcast along
            # the group axis), output DMA chunked to overlap with the DVE ----
            ypad_view = ypad_d[:].rearrange("(g p) f -> p g f", p=128)
            GOUT = 13
            for ci, c0 in enumerate(range(0, NG, GOUT)):
                c1 = min(c0 + GOUT, NG)
                S_b = S_t[:].rearrange("p (a f) -> p a f", a=1).to_broadcast(
                    (128, c1 - c0, OUT)
                )
                T_b = T_t[:].rearrange("p (a f) -> p a f", a=1).to_broadcast(
                    (128, c1 - c0, OUT)
                )
                eng = nc.gpsimd if ci >= 6 else nc.vector
                eng.tensor_mul(
                    hrelu_t[:, c0:c1, :], hrelu_t[:, c0:c1, :], S_b
                )
                eng.tensor_add(
                    hrelu_t[:, c0:c1, :], hrelu_t[:, c0:c1, :], T_b
                )
                nc.sync.dma_start(
                    ypad_view[:, c0:c1, :], hrelu_t[:, c0:c1, :]
                )

    nc.compile()
    return nc


def kernel(x, src, dst, W, b, gamma, beta):
    global LAST_RESULTS
    cfg = CFG
    N, E, IN, OUT, C = cfg["N"], cfg["E"], cfg["IN"], cfg["OUT"], cfg["NCORES"]
    GRP = cfg["GRP"]
    assert x.shape == (N, IN) and W.shape == (IN, OUT)
    assert src.shape == (E,) and dst.shape == (E,)

    meta, gidx_cores, dstoff_cores, wdeg_cores = _preprocess(cfg, src, dst)
    NPC, NG, BROWS = meta["NPC"], meta["NG"], meta["BROWS"]
    XK = _ceil_div(IN, 128)
    last_w = NPC - (NG - 1) * GRP

    nc = _build_nc(cfg, meta)

    import ml_dtypes

    x_bf = np.asarray(x, np.float32).astype(ml_dtypes.bfloat16)
    Wn = np.asarray(W, np.float32)

    iota = np.tile(
        np.arange(GRP, dtype=np.float32)[None, :], (128, 1)
    ).astype(ml_dtypes.bfloat16)
    ident = np.eye(128, dtype=np.float32).astype(ml_dtypes.bfloat16)
    bt = np.tile(np.asarray(b, np.float32)[None, :], (128, 1))
    onesc = np.ones((128, 1), np.float32).astype(ml_dtypes.bfloat16)
    ones8 = np.ones((8, 1), np.float32)
    onest = np.zeros((128, 1), np.float32)
    onest[:last_w] = 1.0
    onest = onest.astype(ml_dtypes.bfloat16)
    onesr = np.ones((1, 128), np.float32)
    gm = np.asarray(gamma, np.float32)[None, :]
    bb = np.asarray(beta, np.float32)[None, :]
    xbanks = [
        np.ascontiguousarray(x_bf[q * BROWS : (q + 1) * BROWS])
        for q in range(cfg["NBANKS"])
    ]
    wtiles = [
        np.ascontiguousarray(Wn[j * 128 : (j + 1) * 128, :]).astype(
            ml_dtypes.bfloat16
        )
        for j in range(XK)
    ]

    in_maps = []
    for k in range(C):
        im = {
            "gidx": gidx_cores[k],
            "doff": dstoff_cores[k],
            "wdeg": wdeg_cores[k],
            "degi": _tile_major(
                meta["deg_in"][k * NPC : (k + 1) * NPC], NG, GRP, np.float32(1.0)
            ),
            "bt": bt,
            "iota": iota,
            "ident": ident,
            "gm": gm,
            "bb": bb,
            "onesc": onesc,
            "ones8": ones8,
            "onest": onest,
            "onesr": onesr,
        }
        for q in range(cfg["NBANKS"]):
            im[f"xb{q}"] = xbanks[q]
        for j in range(XK):
            im[f"wt{j}"] = wtiles[j]
        in_maps.append(im)

    if cfg.get("SIM"):
        from concourse.bass_interp import MultiCoreSim

        sim = MultiCoreSim(nc, num_cores=C)
        for k, core_sim in sim.cores.items():
            for name, val in in_maps[k].items():
                core_sim.tensor(name)[:] = val
        sim.simulate()
        y = np.empty((N, OUT), np.float32)
        for k in range(C):
            y[k * NPC : (k + 1) * NPC] = np.asarray(sim.cores[k].tensor("ypad")[:NPC], dtype=np.float32)
        return y

    global LAST_NC, LAST_RUN_S
    LAST_NC = nc
    import time as _time

    _t0 = _time.time()
    res = bass_utils.run_bass_kernel_spmd(
        nc,
        in_maps,
        core_ids=list(range(C)),
        trace=cfg.get("TRACE", False),
    )
    LAST_RUN_S = _time.time() - _t0
    LAST_RESULTS = res

    y = np.empty((N, OUT), np.float32)
    for k in range(C):
        y[k * NPC : (k + 1) * NPC] = np.asarray(res.results[k]["ypad"][:NPC], dtype=np.float32)
    return y


# revision 7
# speedup vs baseline: 1.3074x; 1.0096x over previous
"""GCN block (GraphConv + BatchNorm1d + ReLU) on 8 Trainium2 NeuronCores.

v2 strategy — "gather x, apply W after aggregation":

By linearity, agg[dst] = sum_e norm_src[src_e] * x[src_e] @ W
                       = (sum_e norm_src[src_e] * x[src_e]) @ W.
So instead of computing h = x@W on every shard and AllGather-ing the h table
(collectives dominated the v1 timeline),每 core receives the FULL x (bf16,
row-major) in its own HBM and directly dma_gathers raw x rows for its edges.
No AllGather at all, and gathers start at t=0. x rows are 256 bf16 = 512 B,
which also clears the <512 B small-descriptor DMA penalty that h rows
(128 bf16 = 256 B) pay.

Per core k (owns dst nodes [k*NPC, (k+1)*NPC)):
  1. For each 128-edge block (bucketed by (dst-group, src-bank)), gather
     x[src] rows (bf16, batched dma_gather) and segment-sum with one-hot
     matmuls Mw^T @ G accumulated in PSUM. The one-hot mask is scaled by
     w_e = rsqrt(deg_out[src_e]) (tensor_scalar is_equal*mult), folding
     norm_src into the aggregation.
  2. Per finished dst group: agg_x [128,256] PSUM -> bf16 SBUF -> PE
     transpose -> (aggT_j)^T @ W_j accumulated -> out [128, OUT] PSUM.
  3. relu(out * rsqrt(clip(deg_in,1)) + b); local BN sums via ones-matmul;
     AllReduce BN sums; y = (h - mu) * rsqrt(var+eps) * gamma + beta.

Host-side work is limited to integer index bookkeeping (bucketing edges by
(core, src-bank, dst-group), degree counting) and layout transforms (bf16
cast, int16 gather indices). All floating-point math runs on device.

Edges are bucketed by src bank (4 banks of N/4 rows) because dma_gather
indices are int16 (< 32768). Bucket sizes are padded to a structure shared
by all 8 cores so a single SPMD NEFF serves every core; pad slots gather row
0 of the bank and carry a dst offset of 255 -> their one-hot column is all
zeros, so they contribute exactly 0. Banks 0 and 3 are padded to >= 1 block
per group so every group starts in bank 0 and stops in bank 3 (keeps the
BN-stat accumulation chain's start first / stop last in program order).
"""
import math
import os
import sys

sys.path.insert(0, "/opt/trn_rl_repo")

import numpy as np

import concourse.bacc as bacc
import concourse.bass as bass
import concourse.mybir as mybir
import concourse.tile as tile
from concourse import bass_utils

F32 = mybir.dt.float32
BF16 = mybir.dt.bfloat16
I16 = mybir.dt.int16

CFG = dict(
    N=100000,
    E=1600000,
    IN=256,
    OUT=128,
    NCORES=8,
    GRP=128,          # dst nodes per segment group (= psum partition dim)
    NBANKS=4,         # src banks (bank rows must stay < 32768 for int16 idx)
    GCHUNK=4,         # dst groups in flight (1 PSUM bank per open accum chain)
    BATCH_BLOCKS=40,  # gather batch size in 128-edge blocks
    EPS=1e-5,
    TRACE=False,
)

LAST_RESULTS = None  # set by kernel() for test harness introspection
LAST_NC = None
LAST_RUN_S = None


def _ceil_div(a, b):
    return (a + b - 1) // b


def _wrap16(idx, ncols):
    """int16 idx list -> [128, ncols] tile: idx i at [i%16, i//16], replicated
    8x across the 16-partition groups (one copy per GpSimd Q7 core)."""
    n = idx.shape[0]
    assert n == ncols * 16
    w = np.ascontiguousarray(idx.reshape(ncols, 16).T)
    return np.tile(w, (8, 1))


def _preprocess(cfg, src, dst):
    """Bucket edges by (owner core, src bank, dst group); build per-core
    gather-index / dst-offset / src-degree arrays and the shared block
    structure."""
    N, E = cfg["N"], cfg["E"]
    C, NBANKS, GRP = cfg["NCORES"], cfg["NBANKS"], cfg["GRP"]
    NPC = N // C
    NG = _ceil_div(NPC, GRP)
    assert N % NBANKS == 0
    BROWS = N // NBANKS            # rows per x bank (gather source table)
    assert BROWS < 32768

    src = src.astype(np.int64)
    dst = dst.astype(np.int64)
    deg_out = np.bincount(src, minlength=N).astype(np.float32)
    deg_in = np.bincount(dst, minlength=N).astype(np.float32)

    owner = dst // NPC
    bank = src // BROWS
    grp = (dst % NPC) // GRP
    key = (owner * NBANKS + bank) * NG + grp
    order = np.argsort(key, kind="stable")
    s_src = src[order]
    s_dst = dst[order]
    s_key = key[order]

    counts = np.bincount(key, minlength=C * NBANKS * NG).reshape(C, NBANKS, NG)
    # bucket capacity: exact max over cores (shared SPMD structure); >= 1 in
    # banks 0/3 so every group has a first (bank-0) and last (bank-3) matmul
    P = counts.max(axis=0)  # [NBANKS, NG]
    P[0] = np.maximum(P[0], 1)
    P[NBANKS - 1] = np.maximum(P[NBANKS - 1], 1)

    # stream order: group-chunks outer, banks inner; a group's PSUM slot is
    # live across all banks of its chunk (accumulated with start/stop).
    # Segment (chunk, bank) = that chunk's buckets concatenated, padded to a
    # multiple of 128; 128-edge blocks may straddle bucket (group) boundaries
    # -> one matmul per (block, overlapped group).
    GC = cfg["GCHUNK"]
    chunks = [list(range(c, min(c + GC, NG))) for c in range(0, NG, GC)]
    run_off = np.zeros((NBANKS, NG), np.int64)
    segments = []  # (bank, seg_start_slot, seg_nblocks)
    pos = 0
    for ch in chunks:
        for b in range(NBANKS):
            seg0 = pos
            for g in ch:
                run_off[b, g] = pos
                pos += P[b, g]
            pos = ((pos + 127) // 128) * 128  # segment tail pad
            segments.append((b, seg0, (pos - seg0) // 128))
    nidx_tot = pos
    nb_tot = nidx_tot // 128

    # per block: list of matmuls (mm_col, group); per group: first/last mm id
    block_mms = [[] for _ in range(nb_tot)]
    n_mm = 0
    mm_of_group = {}
    for ch in chunks:
        for b in range(NBANKS):
            for g in ch:
                o0, o1 = run_off[b, g], run_off[b, g] + P[b, g]
                for t in range(o0 // 128, (o1 - 1) // 128 + 1):
                    block_mms[t].append((n_mm, g))
                    mm_of_group.setdefault(g, []).append(n_mm)
                    n_mm += 1
    mm_flags = {}
    for g, mms in mm_of_group.items():
        for m in mms:
            mm_flags[m] = (m == mms[0], m == mms[-1])

    # boundaries of each (k, b, g) bucket in the sorted edge stream
    bkeys = (np.arange(C)[:, None, None] * NBANKS + np.arange(NBANKS)[None, :, None]) * NG + np.arange(NG)[None, None, :]
    starts = np.searchsorted(s_key, bkeys.ravel()).reshape(C, NBANKS, NG)
    ends = np.searchsorted(s_key, bkeys.ravel(), side="right").reshape(C, NBANKS, NG)

    gidx_cores = []
    dstoff_cores = []
    wdeg_cores = []
    for k in range(C):
        gidx = np.zeros(nidx_tot, np.int16)
        doff = np.full((n_mm, 128), 255.0, np.float32)
        wdeg = np.ones((n_mm, 128), np.float32)
        # fill gather indices per bucket (pad slots keep row 0)
        for b in range(NBANKS):
            for g in range(NG):
                s, e = starts[k, b, g], ends[k, b, g]
                cnt = e - s
                if cnt == 0:
                    continue
                p0 = run_off[b, g]
                gidx[p0 : p0 + cnt] = (s_src[s:e] % BROWS).astype(np.int16)
        # fill per-matmul mask columns: rows = this core's real edges of the
        # matmul's group that fall inside the block's 128-slot window
        for ch in chunks:
            for b in range(NBANKS):
                for g in ch:
                    s, e = starts[k, b, g], ends[k, b, g]
                    cnt = e - s
                    o0 = run_off[b, g]
                    bucket_mms = [
                        (m, t)
                        for t in range(o0 // 128, (o0 + P[b, g] - 1) // 128 + 1)
                        for (m, gg) in block_mms[t]
                        if gg == g
                    ]
                    if cnt == 0:
                        continue
                    dvals = ((s_dst[s:e] % NPC) - g * GRP).astype(np.float32)
                    wvals = deg_out[s_src[s:e]]
                    for m, t in bucket_mms:
                        w0 = t * 128
                        lo = max(o0, w0)
                        hi = min(o0 + cnt, w0 + 128)
                        if lo >= hi:
                            continue
                        rows = np.arange(lo - w0, hi - w0)
                        doff[m, rows] = dvals[lo - o0 : hi - o0]
                        wdeg[m, rows] = wvals[lo - o0 : hi - o0]
        gidx_cores.append(_wrap16(gidx, nidx_tot // 16))
        dstoff_cores.append(np.ascontiguousarray(doff.T))
        wdeg_cores.append(np.ascontiguousarray(wdeg.T))

    # gather batches: split segments longer than BATCH_BLOCKS
    batches = []  # (bank, first_block, n_blocks)
    for b, seg0, nblk in segments:
        t = seg0 // 128
        left = nblk
        while left > 0:
            n = min(left, cfg["BATCH_BLOCKS"])
            batches.append((b, t, n))
            t += n
            left -= n

    meta = dict(
        NPC=NPC,
        NG=NG,
        BROWS=BROWS,
        nidx_tot=nidx_tot,
        nb_tot=nb_tot,
        n_mm=n_mm,
        block_mms=block_mms,
        mm_flags=mm_flags,
        batches=batches,
        deg_in=deg_in,
    )
    return meta, gidx_cores, dstoff_cores, wdeg_cores


def _tile_major(vec, NG, GRP, pad_val):
    """[NPC] -> [GRP, NG]: entry (p, m) = vec[m*GRP + p], padded."""
    out = np.full((NG * GRP,), pad_val, vec.dtype)
    out[: vec.shape[0]] = vec
    return np.ascontiguousarray(out.reshape(NG, GRP).T)


def _build_nc(cfg, meta):
    N, IN, OUT, C = cfg["N"], cfg["IN"], cfg["OUT"], cfg["NCORES"]
    GRP, NBANKS, GC = cfg["GRP"], cfg["NBANKS"], cfg["GCHUNK"]
    NPC, NG, BROWS = meta["NPC"], meta["NG"], meta["BROWS"]
    nidx_tot, nb_tot = meta["nidx_tot"], meta["nb_tot"]
    n_mm = meta["n_mm"]
    block_mms, mm_flags = meta["block_mms"], meta["mm_flags"]
    batches = meta["batches"]
    XK = _ceil_div(IN, 128)
    assert OUT == 128 and GRP == 128 and XK == 2
    last_w = NPC - (NG - 1) * GRP  # valid rows in the last group

    nc = bacc.Bacc(
        "TRN2", target_bir_lowering=False, debug=False, num_devices=C
    )

    # ---- external inputs ----
    xb = [
        nc.dram_tensor(f"xb{q}", [BROWS, IN], BF16, kind="ExternalInput")
        for q in range(NBANKS)
    ]
    wt = [
        nc.dram_tensor(f"wt{j}", [128, OUT], BF16, kind="ExternalInput")
        for j in range(XK)
    ]
    gidx_d = nc.dram_tensor("gidx", [128, nidx_tot // 16], I16, kind="ExternalInput")
    doff_d = nc.dram_tensor("doff", [128, n_mm], F32, kind="ExternalInput")
    wdeg_d = nc.dram_tensor("wdeg", [128, n_mm], F32, kind="ExternalInput")
    degi_d = nc.dram_tensor("degi", [128, NG], F32, kind="ExternalInput")
    bt_d = nc.dram_tensor("bt", [128, OUT], F32, kind="ExternalInput")
    iota_d = nc.dram_tensor("iota", [128, GRP], BF16, kind="ExternalInput")
    ident_d = nc.dram_tensor("ident", [128, 128], BF16, kind="ExternalInput")
    gm_d = nc.dram_tensor("gm", [1, OUT], F32, kind="ExternalInput")
    bb_d = nc.dram_tensor("bb", [1, OUT], F32, kind="ExternalInput")
    onesc_d = nc.dram_tensor("onesc", [128, 1], BF16, kind="ExternalInput")
    ones8_d = nc.dram_tensor("ones8", [8, 1], F32, kind="ExternalInput")
    onest_d = nc.dram_tensor("onest", [128, 1], BF16, kind="ExternalInput")
    onesr_d = nc.dram_tensor("onesr", [1, 128], F32, kind="ExternalInput")

    ypad_d = nc.dram_tensor("ypad", [NG * GRP, OUT], BF16, kind="ExternalOutput")

    with tile.TileContext(nc) as tc:
        with (
            tc.tile_pool(name="const", bufs=1) as cpool,
            tc.tile_pool(name="dram", bufs=1, space="DRAM") as dpool,
            tc.tile_pool(name="hrelu", bufs=1) as hpool,
            tc.tile_pool(name="gath", bufs=5) as gpool,
            tc.tile_pool(name="mpool", bufs=10) as mpool,
            tc.tile_pool(name="capool", bufs=6) as capool,
            tc.tile_pool(name="etmp", bufs=6) as epool,
            tc.tile_pool(name="gtmp", bufs=4) as gpool2,
            tc.tile_pool(name="psagg", bufs=1, space="PSUM") as pagg,
            tc.tile_pool(name="psaux", bufs=1, space="PSUM") as paux,
            tc.tile_pool(name="pstat", bufs=1, space="PSUM") as pspool,
        ):
            # ---- constants / small tiles ----
            iota_t = cpool.tile([128, GRP], BF16)
            ident_t = cpool.tile([128, 128], BF16)
            bt_t = cpool.tile([128, OUT], F32)
            degi_t = cpool.tile([128, NG], F32)
            ndst_t = cpool.tile([128, NG], F32)
            gm_t = cpool.tile([1, OUT], F32)
            bb_t = cpool.tile([1, OUT], F32)
            onesc_t = cpool.tile([128, 1], BF16)
            ones8_t = cpool.tile([8, 1], F32)
            onest_t = cpool.tile([128, 1], BF16)
            onesr_t = cpool.tile([1, 128], F32)
            gidx_t = cpool.tile([128, nidx_tot // 16], I16)
            doff_t = cpool.tile([128, n_mm], F32)
            wsrc_t = cpool.tile([128, n_mm], F32)
            wts = [cpool.tile([128, OUT], BF16, name=f"wt_s{j}") for j in range(XK)]

            # split the big index loads so the first gathers aren't gated on
            # the full-table DMA
            gsplit = min(nidx_tot // 16, 256)
            nc.sync.dma_start(gidx_t[:, :gsplit], gidx_d[:, :gsplit])
            if gsplit < nidx_tot // 16:
                nc.sync.dma_start(gidx_t[:, gsplit:], gidx_d[:, gsplit:])
            dsplit = min(n_mm, 128)
            nc.sync.dma_start(doff_t[:, :dsplit], doff_d[:, :dsplit])
            nc.sync.dma_start(wsrc_t[:, :dsplit], wdeg_d[:, :dsplit])
            if dsplit < n_mm:
                nc.sync.dma_start(doff_t[:, dsplit:], doff_d[:, dsplit:])
                nc.sync.dma_start(wsrc_t[:, dsplit:], wdeg_d[:, dsplit:])
            nc.sync.dma_start(iota_t[:], iota_d[:])
            nc.sync.dma_start(ident_t[:], ident_d[:])
            nc.sync.dma_start(bt_t[:], bt_d[:])
            nc.sync.dma_start(degi_t[:], degi_d[:])
            nc.sync.dma_start(gm_t[:], gm_d[:])
            nc.sync.dma_start(bb_t[:], bb_d[:])
            nc.sync.dma_start(onesc_t[:], onesc_d[:])
            nc.sync.dma_start(ones8_t[:], ones8_d[:])
            nc.sync.dma_start(onest_t[:], onest_d[:])
            nc.sync.dma_start(onesr_t[:], onesr_d[:])
            for j in range(XK):
                nc.sync.dma_start(wts[j][:], wt[j][:])

            # per-edge src norm: w = rsqrt(deg_out[src]) (pad slots carry 1.0);
            # two pieces so the first masks aren't gated on the full tile
            for c0, c1 in ((0, dsplit), (dsplit, n_mm)):
                if c0 >= c1:
                    continue
                nc.vector.reciprocal(wsrc_t[:, c0:c1], wsrc_t[:, c0:c1])
                nc.scalar.activation(
                    wsrc_t[:, c0:c1], wsrc_t[:, c0:c1],
                    mybir.ActivationFunctionType.Sqrt,
                )
            # dst norm: rsqrt(max(deg_in, 1)) tile-major [GRP, NG]
            nc.vector.tensor_scalar(
                ndst_t[:], degi_t[:], 1.0, None, op0=mybir.AluOpType.max
            )
            nc.vector.reciprocal(ndst_t[:], ndst_t[:])
            nc.scalar.activation(
                ndst_t[:], ndst_t[:], mybir.ActivationFunctionType.Sqrt
            )

            stats_in = dpool.tile([1, 2 * OUT], F32)
            _aspace = "Local" if cfg.get("NOCC") else "Shared"
            stats_out = dpool.tile([C, 2 * OUT], F32, addr_space=_aspace)

            hrelu_t = hpool.tile([128, NG, OUT], BF16)

            # ---- PSUM layout (8 banks x 2KB); accumulation-group zeroing is
            # bank-granular, so every concurrently-open chain gets its own
            # bank: 4x agg (GCHUNK groups in flight) + 1x transpose + 2x out
            # (alternating, WAR-tracked) + 1x BN stats (sum+sq as one chain).
            assert GC == 4
            # agg tiles carry a 512B tail used (bitcast bf16) as the per-group
            # transpose target, so each group's finish chain owns its own bank
            ps_agg = [
                pagg.tile([128, 2 * OUT + 128], F32, name=f"ps_agg{i}")
                for i in range(GC)
            ]
            ps_out = [
                paux.tile([128, OUT], F32, name=f"ps_out{i}") for i in range(3)
            ]
            ps_stat = pspool.tile([1, 2, OUT], F32, name="ps_stat")

            # ---- stage D: gather x rows + one-hot matmul segmented sum ----
            def _finish_group(g):
                """Group g's PSUM agg is complete: apply W, relu, BN partials."""
                gi = g % GC
                cagg = capool.tile([128, 2 * OUT], BF16, tag="cagg")
                nc.scalar.activation(
                    cagg[:], ps_agg[gi][:, : 2 * OUT],
                    mybir.ActivationFunctionType.Copy,
                )
                # transpose into this group's own agg bank tail (bitcast bf16)
                # so concurrent groups' finish chains don't share a PSUM bank
                trv = ps_agg[gi][:, 2 * OUT :].bitcast(BF16)
                for h in range(2):
                    nc.tensor.matmul(
                        trv[:, h * OUT : (h + 1) * OUT],
                        cagg[:, h * OUT : (h + 1) * OUT],
                        ident_t[:],
                        is_transpose=True,
                        start=(h == 0),
                        stop=(h == 1),
                    )
                tagg = capool.tile([128, 2, OUT], BF16, tag="tagg")
                nc.scalar.activation(
                    tagg[:, :, :],
                    trv[:].rearrange("p (a f) -> p a f", a=2),
                    mybir.ActivationFunctionType.Copy,
                )
                po = ps_out[g % 3]
                for jj in range(XK):
                    nc.tensor.matmul(
                        po[:],
                        tagg[:, jj, :],
                        wts[jj][:],
                        start=(jj == 0),
                        stop=(jj == XK - 1),
                    )
                # stage E: relu(out * ndst + b), BN partial sums
                tmp = epool.tile([128, OUT], F32, tag="etmp")
                nc.vector.scalar_tensor_tensor(
                    tmp[:],
                    po[:],
                    ndst_t[:, g : g + 1],
                    bt_t[:],
                    op0=mybir.AluOpType.mult,
                    op1=mybir.AluOpType.add,
                )
                nc.scalar.activation(
                    hrelu_t[:, g, :], tmp[:], mybir.ActivationFunctionType.Relu
                )
                ones = onesc_t if g < NG - 1 else onest_t
                nc.tensor.matmul(
                    ps_stat[:, 0, :],
                    ones[:],
                    hrelu_t[:, g, :],
                    start=(g == 0),
                    stop=False,
                )
                sq = epool.tile([128, OUT], BF16, tag="esq")
                nc.scalar.activation(
                    sq[:], hrelu_t[:, g, :], mybir.ActivationFunctionType.Square
                )
                nc.tensor.matmul(
                    ps_stat[:, 1, :],
                    ones[:],
                    sq[:],
                    start=False,
                    stop=(g == NG - 1),
                )

            bmax = max(nb for _, _, nb in batches)
            for bank, t0, nblk in batches:
                Gt = gpool.tile([128, bmax, IN], BF16, tag="G")
                nc.gpsimd.dma_gather(
                    Gt[:, :nblk, :],
                    xb[bank][:],
                    gidx_t[:, t0 * 8 : (t0 + nblk) * 8],
                    nblk * 128,
                    nblk * 128,
                    IN,
                    single_packet=False,
                )
                for j in range(nblk):
                    t = t0 + j
                    for m, g in block_mms[t]:
                      is_start, is_stop = mm_flags[m]
                      gi = g % GC
                      Mt = mpool.tile([128, GRP], BF16, tag="M")
                      nc.vector.tensor_scalar(
                          Mt[:],
                          iota_t[:],
                          doff_t[:, m : m + 1],
                          wsrc_t[:, m : m + 1],
                          op0=mybir.AluOpType.is_equal,
                          op1=mybir.AluOpType.mult,
                      )
                      nc.tensor.matmul(
                          ps_agg[gi][:, : 2 * OUT],
                          Mt[:],
                          Gt[:, j, :],
                          start=is_start,
                          stop=is_stop,
                      )
                      if is_stop:
                          _finish_group(g)

            # ---- stage F: AllReduce BN stats; build affine S/T tiles ----
            S_t = cpool.tile([128, OUT], BF16)
            T_t = cpool.tile([128, OUT], BF16)
            st_sb = cpool.tile([1, 2 * OUT], F32)
            nc.scalar.activation(
                st_sb[:], ps_stat[:].rearrange("p a f -> p (a f)"),
                mybir.ActivationFunctionType.Copy,
            )
            nc.sync.dma_start(stats_in[:], st_sb[:])
            if cfg.get("NOCC"):
                stats_out = stats_in  # single-core debug: sums are the totals
                st8 = st_sb
                ones8v = None
            else:
                # AllGather (no 1.875x reduce multiplier) + tiny local
                # ones-matmul reduction beats AllReduce on latency
                nc.gpsimd.collective_compute(
                    "AllGather",
                    mybir.AluOpType.bypass,
                    replica_groups=[list(range(C))],
                    ins=[stats_in[:]],
                    outs=[stats_out[:]],
                )
                st8 = cpool.tile([C, 2 * OUT], F32)
                nc.sync.dma_start(st8[:], stats_out[:])
                ones8v = ones8_t
            st_rb = cpool.tile([1, 2 * OUT], F32)
            if ones8v is None:
                nc.scalar.activation(
                    st_rb[:], st8[:], mybir.ActivationFunctionType.Copy
                )
            else:
                ps_red = ps_stat[:].rearrange("p a f -> p (a f)")
                nc.tensor.matmul(ps_red, ones8v[:], st8[:], start=True, stop=True)
                nc.scalar.activation(
                    st_rb[:], ps_red, mybir.ActivationFunctionType.Copy
                )

            mu = cpool.tile([1, OUT], F32)
            musq = cpool.tile([1, OUT], F32)
            var = cpool.tile([1, OUT], F32)
            srow = cpool.tile([1, OUT], F32)
            trow = cpool.tile([1, OUT], F32)
            inv_n = 1.0 / float(N)
            nc.scalar.activation(
                mu[:], st_rb[:, 0:OUT], mybir.ActivationFunctionType.Copy, scale=inv_n
            )
            nc.scalar.activation(
                musq[:], mu[:], mybir.ActivationFunctionType.Square
            )
            # var + eps = (E[x^2]*inv_n + eps) - mu^2, then rsqrt via
            # reciprocal+sqrt (ACT Rsqrt is banned for accuracy)
            nc.scalar.activation(
                st_rb[:, OUT : 2 * OUT],
                st_rb[:, OUT : 2 * OUT],
                mybir.ActivationFunctionType.Copy,
                scale=inv_n,
                bias=float(cfg["EPS"]),
            )
            nc.vector.tensor_sub(var[:], st_rb[:, OUT : 2 * OUT], musq[:])
            nc.vector.reciprocal(var[:], var[:])
            nc.scalar.activation(
                var[:], var[:], mybir.ActivationFunctionType.Sqrt
            )
            nc.vector.tensor_mul(srow[:], gm_t[:], var[:])
            nc.vector.tensor_mul(trow[:], mu[:], srow[:])
            nc.vector.tensor_sub(trow[:], bb_t[:], trow[:])

            # reuse the (now idle) out banks for the S/T broadcast matmuls
            nc.tensor.matmul(ps_out[0][:], onesr_t[:], srow[:], start=True, stop=True)
            nc.scalar.activation(
                S_t[:], ps_out[0][:], mybir.ActivationFunctionType.Copy
            )
            nc.tensor.matmul(ps_out[1][:], onesr_t[:], trow[:], start=True, stop=True)
            nc.scalar.activation(
                T_t[:], ps_out[1][:], mybir.ActivationFunctionType.Copy
            )

            # ---- stage G: y = hrelu * S + T (in place, S/T broadcast along
            # the group axis), output DMA chunked to overlap with the DVE ----
            ypad_view = ypad_d[:].rearrange("(g p) f -> p g f", p=128)
            GOUT = 13
            for ci, c0 in enumerate(range(0, NG, GOUT)):
                c1 = min(c0 + GOUT, NG)
                S_b = S_t[:].rearrange("p (a f) -> p a f", a=1).to_broadcast(
                    (128, c1 - c0, OUT)
                )
                T_b = T_t[:].rearrange("p (a f) -> p a f", a=1).to_broadcast(
                    (128, c1 - c0, OUT)
                )
                eng = nc.gpsimd if ci >= 6 else nc.vector
                eng.tensor_mul(
                    hrelu_t[:, c0:c1, :], hrelu_t[:, c0:c1, :], S_b
                )
                eng.tensor_add(
                    hrelu_t[:, c0:c1, :], hrelu_t[:, c0:c1, :], T_b
                )
                nc.sync.dma_start(
                    ypad_view[:, c0:c1, :], hrelu_t[:, c0:c1, :]
                )

    nc.compile()
    return nc


def kernel(x, src, dst, W, b, gamma, beta):
    global LAST_RESULTS
    cfg = CFG
    N, E, IN, OUT, C = cfg["N"], cfg["E"], cfg["IN"], cfg["OUT"], cfg["NCORES"]
    GRP = cfg["GRP"]
    assert x.shape == (N, IN) and W.shape == (IN, OUT)
    assert src.shape == (E,) and dst.shape == (E,)

    meta, gidx_cores, dstoff_cores, wdeg_cores = _preprocess(cfg, src, dst)
    NPC, NG, BROWS = meta["NPC"], meta["NG"], meta["BROWS"]
    XK = _ceil_div(IN, 128)
    last_w = NPC - (NG - 1) * GRP

    nc = _build_nc(cfg, meta)

    import ml_dtypes

    x_bf = np.asarray(x, np.float32).astype(ml_dtypes.bfloat16)
    Wn = np.asarray(W, np.float32)

    iota = np.tile(
        np.arange(GRP, dtype=np.float32)[None, :], (128, 1)
    ).astype(ml_dtypes.bfloat16)
    ident = np.eye(128, dtype=np.float32).astype(ml_dtypes.bfloat16)
    bt = np.tile(np.asarray(b, np.float32)[None, :], (128, 1))
    onesc = np.ones((128, 1), np.float32).astype(ml_dtypes.bfloat16)
    ones8 = np.ones((8, 1), np.float32)
    onest = np.zeros((128, 1), np.float32)
    onest[:last_w] = 1.0
    onest = onest.astype(ml_dtypes.bfloat16)
    onesr = np.ones((1, 128), np.float32)
    gm = np.asarray(gamma, np.float32)[None, :]
    bb = np.asarray(beta, np.float32)[None, :]
    xbanks = [
        np.ascontiguousarray(x_bf[q * BROWS : (q + 1) * BROWS])
        for q in range(cfg["NBANKS"])
    ]
    wtiles = [
        np.ascontiguousarray(Wn[j * 128 : (j + 1) * 128, :]).astype(
            ml_dtypes.bfloat16
        )
        for j in range(XK)
    ]

    in_maps = []
    for k in range(C):
        im = {
            "gidx": gidx_cores[k],
            "doff": dstoff_cores[k],
            "wdeg": wdeg_cores[k],
            "degi": _tile_major(
                meta["deg_in"][k * NPC : (k + 1) * NPC], NG, GRP, np.float32(1.0)
            ),
            "bt": bt,
            "iota": iota,
            "ident": ident,
            "gm": gm,
            "bb": bb,
            "onesc": onesc,
            "ones8": ones8,
            "onest": onest,
            "onesr": onesr,
        }
        for q in range(cfg["NBANKS"]):
            im[f"xb{q}"] = xbanks[q]
        for j in range(XK):
            im[f"wt{j}"] = wtiles[j]
        in_maps.append(im)

    if cfg.get("SIM"):
        from concourse.bass_interp import MultiCoreSim

        sim = MultiCoreSim(nc, num_cores=C)
        for k, core_sim in sim.cores.items():
            for name, val in in_maps[k].items():
                core_sim.tensor(name)[:] = val
        sim.simulate()
        y = np.empty((N, OUT), np.float32)
        for k in range(C):
            y[k * NPC : (k + 1) * NPC] = np.asarray(sim.cores[k].tensor("ypad")[:NPC], dtype=np.float32)
        return y

    global LAST_NC, LAST_RUN_S
    LAST_NC = nc
    import time as _time

    _t0 = _time.time()
    res = bass_utils.run_bass_kernel_spmd(
        nc,
        in_maps,
        core_ids=list(range(C)),
        trace=cfg.get("TRACE", False),
    )
    LAST_RUN_S = _time.time() - _t0
    LAST_RESULTS = res

    y = np.empty((N, OUT), np.float32)
    for k in range(C):
        y[k * NPC : (k + 1) * NPC] = np.asarray(res.results[k]["ypad"][:NPC], dtype=np.float32)
    return y


# revision 8
# speedup vs baseline: 1.3123x; 1.0038x over previous
"""GCN block (GraphConv + BatchNorm1d + ReLU) on 8 Trainium2 NeuronCores.

v2 strategy — "gather x, apply W after aggregation":

By linearity, agg[dst] = sum_e norm_src[src_e] * x[src_e] @ W
                       = (sum_e norm_src[src_e] * x[src_e]) @ W.
So instead of computing h = x@W on every shard and AllGather-ing the h table
(collectives dominated the v1 timeline),每 core receives the FULL x (bf16,
row-major) in its own HBM and directly dma_gathers raw x rows for its edges.
No AllGather at all, and gathers start at t=0. x rows are 256 bf16 = 512 B,
which also clears the <512 B small-descriptor DMA penalty that h rows
(128 bf16 = 256 B) pay.

Per core k (owns dst nodes [k*NPC, (k+1)*NPC)):
  1. For each 128-edge block (bucketed by (dst-group, src-bank)), gather
     x[src] rows (bf16, batched dma_gather) and segment-sum with one-hot
     matmuls Mw^T @ G accumulated in PSUM. The one-hot mask is scaled by
     w_e = rsqrt(deg_out[src_e]) (tensor_scalar is_equal*mult), folding
     norm_src into the aggregation.
  2. Per finished dst group: agg_x [128,256] PSUM -> bf16 SBUF -> PE
     transpose -> (aggT_j)^T @ W_j accumulated -> out [128, OUT] PSUM.
  3. relu(out * rsqrt(clip(deg_in,1)) + b); local BN sums via ones-matmul;
     AllReduce BN sums; y = (h - mu) * rsqrt(var+eps) * gamma + beta.

Host-side work is limited to integer index bookkeeping (bucketing edges by
(core, src-bank, dst-group), degree counting) and layout transforms (bf16
cast, int16 gather indices). All floating-point math runs on device.

Edges are bucketed by src bank (4 banks of N/4 rows) because dma_gather
indices are int16 (< 32768). Bucket sizes are padded to a structure shared
by all 8 cores so a single SPMD NEFF serves every core; pad slots gather row
0 of the bank and carry a dst offset of 255 -> their one-hot column is all
zeros, so they contribute exactly 0. Banks 0 and 3 are padded to >= 1 block
per group so every group starts in bank 0 and stops in bank 3 (keeps the
BN-stat accumulation chain's start first / stop last in program order).
"""
import math
import os
import sys

sys.path.insert(0, "/opt/trn_rl_repo")

import numpy as np

import concourse.bacc as bacc
import concourse.bass as bass
import concourse.mybir as mybir
import concourse.tile as tile
from concourse import bass_utils

F32 = mybir.dt.float32
BF16 = mybir.dt.bfloat16
I16 = mybir.dt.int16

CFG = dict(
    N=100000,
    E=1600000,
    IN=256,
    OUT=128,
    NCORES=8,
    GRP=128,          # dst nodes per segment group (= psum partition dim)
    NBANKS=4,         # src banks (bank rows must stay < 32768 for int16 idx)
    GCHUNK=4,         # dst groups in flight (1 PSUM bank per open accum chain)
    BATCH_BLOCKS=40,  # gather batch size in 128-edge blocks
    EPS=1e-5,
    TRACE=False,
)

LAST_RESULTS = None  # set by kernel() for test harness introspection
LAST_NC = None
LAST_RUN_S = None


def _ceil_div(a, b):
    return (a + b - 1) // b


def _wrap16(idx, ncols):
    """int16 idx list -> [128, ncols] tile: idx i at [i%16, i//16], replicated
    8x across the 16-partition groups (one copy per GpSimd Q7 core)."""
    n = idx.shape[0]
    assert n == ncols * 16
    w = np.ascontiguousarray(idx.reshape(ncols, 16).T)
    return np.tile(w, (8, 1))


def _preprocess(cfg, src, dst):
    """Bucket edges by (owner core, src bank, dst group); build per-core
    gather-index / dst-offset / src-degree arrays and the shared block
    structure."""
    N, E = cfg["N"], cfg["E"]
    C, NBANKS, GRP = cfg["NCORES"], cfg["NBANKS"], cfg["GRP"]
    NPC = N // C
    NG = _ceil_div(NPC, GRP)
    assert N % NBANKS == 0
    BROWS = N // NBANKS            # rows per x bank (gather source table)
    assert BROWS < 32768

    src = src.astype(np.int64)
    dst = dst.astype(np.int64)
    deg_out = np.bincount(src, minlength=N).astype(np.float32)
    deg_in = np.bincount(dst, minlength=N).astype(np.float32)
    bank = src // BROWS

    # permute dst nodes into (core, group, slot) positions so that the 8
    # cores' group-i buckets have near-identical per-bank in-edge counts --
    # shrinks the SPMD max-over-cores padding of the shared block structure.
    # Nodes sorted by per-bank count vector are dealt round-robin across
    # cores at each (group, slot).
    nbc = np.zeros((N, NBANKS), np.int64)
    for b in range(NBANKS):
        nbc[:, b] = np.bincount(dst[bank == b], minlength=N)
    node_order = np.lexsort(tuple(-nbc[:, b] for b in range(NBANKS - 1, -1, -1)))
    allpos = np.arange(N)
    deal_order = np.lexsort((allpos // NPC, allpos % GRP, (allpos % NPC) // GRP))
    perm = np.empty(N, np.int64)      # new position -> original node
    perm[deal_order] = node_order
    pos = np.empty(N, np.int64)       # original node -> new position
    pos[perm] = allpos

    pdst = pos[dst]
    owner = pdst // NPC
    grp = (pdst % NPC) // GRP
    key = (owner * NBANKS + bank) * NG + grp
    order = np.argsort(key, kind="stable")
    s_src = src[order]
    s_pdst = pos[dst[order]]
    s_key = key[order]

    counts = np.bincount(key, minlength=C * NBANKS * NG).reshape(C, NBANKS, NG)
    # bucket capacity: exact max over cores (shared SPMD structure); >= 1 in
    # banks 0/3 so every group has a first (bank-0) and last (bank-3) matmul
    P = counts.max(axis=0)  # [NBANKS, NG]
    P[0] = np.maximum(P[0], 1)
    P[NBANKS - 1] = np.maximum(P[NBANKS - 1], 1)

    # stream order: group-chunks outer, banks inner; a group's PSUM slot is
    # live across all banks of its chunk (accumulated with start/stop).
    # Segment (chunk, bank) = that chunk's buckets concatenated, padded to a
    # multiple of 128; 128-edge blocks may straddle bucket (group) boundaries
    # -> one matmul per (block, overlapped group).
    GC = cfg["GCHUNK"]
    chunks = [list(range(c, min(c + GC, NG))) for c in range(0, NG, GC)]
    run_off = np.zeros((NBANKS, NG), np.int64)
    segments = []  # (bank, seg_start_slot, seg_nblocks)
    pos = 0
    for ch in chunks:
        for b in range(NBANKS):
            seg0 = pos
            for g in ch:
                run_off[b, g] = pos
                pos += P[b, g]
            pos = ((pos + 127) // 128) * 128  # segment tail pad
            segments.append((b, seg0, (pos - seg0) // 128))
    nidx_tot = pos
    nb_tot = nidx_tot // 128

    # per block: list of matmuls (mm_col, group); per group: first/last mm id
    block_mms = [[] for _ in range(nb_tot)]
    n_mm = 0
    mm_of_group = {}
    for ch in chunks:
        for b in range(NBANKS):
            for g in ch:
                o0, o1 = run_off[b, g], run_off[b, g] + P[b, g]
                for t in range(o0 // 128, (o1 - 1) // 128 + 1):
                    block_mms[t].append((n_mm, g))
                    mm_of_group.setdefault(g, []).append(n_mm)
                    n_mm += 1
    mm_flags = {}
    for g, mms in mm_of_group.items():
        for m in mms:
            mm_flags[m] = (m == mms[0], m == mms[-1])

    # boundaries of each (k, b, g) bucket in the sorted edge stream
    bkeys = (np.arange(C)[:, None, None] * NBANKS + np.arange(NBANKS)[None, :, None]) * NG + np.arange(NG)[None, None, :]
    starts = np.searchsorted(s_key, bkeys.ravel()).reshape(C, NBANKS, NG)
    ends = np.searchsorted(s_key, bkeys.ravel(), side="right").reshape(C, NBANKS, NG)

    gidx_cores = []
    dstoff_cores = []
    wdeg_cores = []
    for k in range(C):
        gidx = np.zeros(nidx_tot, np.int16)
        doff = np.full((n_mm, 128), 255.0, np.float32)
        wdeg = np.ones((n_mm, 128), np.float32)
        # fill gather indices per bucket (pad slots keep row 0)
        for b in range(NBANKS):
            for g in range(NG):
                s, e = starts[k, b, g], ends[k, b, g]
                cnt = e - s
                if cnt == 0:
                    continue
                p0 = run_off[b, g]
                gidx[p0 : p0 + cnt] = (s_src[s:e] % BROWS).astype(np.int16)
        # fill per-matmul mask columns: rows = this core's real edges of the
        # matmul's group that fall inside the block's 128-slot window
        for ch in chunks:
            for b in range(NBANKS):
                for g in ch:
                    s, e = starts[k, b, g], ends[k, b, g]
                    cnt = e - s
                    o0 = run_off[b, g]
                    bucket_mms = [
                        (m, t)
                        for t in range(o0 // 128, (o0 + P[b, g] - 1) // 128 + 1)
                        for (m, gg) in block_mms[t]
                        if gg == g
                    ]
                    if cnt == 0:
                        continue
                    dvals = ((s_pdst[s:e] % NPC) - g * GRP).astype(np.float32)
                    wvals = deg_out[s_src[s:e]]
                    for m, t in bucket_mms:
                        w0 = t * 128
                        lo = max(o0, w0)
                        hi = min(o0 + cnt, w0 + 128)
                        if lo >= hi:
                            continue
                        rows = np.arange(lo - w0, hi - w0)
                        doff[m, rows] = dvals[lo - o0 : hi - o0]
                        wdeg[m, rows] = wvals[lo - o0 : hi - o0]
        gidx_cores.append(_wrap16(gidx, nidx_tot // 16))
        dstoff_cores.append(np.ascontiguousarray(doff.T))
        wdeg_cores.append(np.ascontiguousarray(wdeg.T))

    # gather batches: split segments longer than BATCH_BLOCKS
    batches = []  # (bank, first_block, n_blocks)
    for b, seg0, nblk in segments:
        t = seg0 // 128
        left = nblk
        while left > 0:
            n = min(left, cfg["BATCH_BLOCKS"])
            batches.append((b, t, n))
            t += n
            left -= n

    meta = dict(
        NPC=NPC,
        NG=NG,
        BROWS=BROWS,
        nidx_tot=nidx_tot,
        nb_tot=nb_tot,
        n_mm=n_mm,
        block_mms=block_mms,
        mm_flags=mm_flags,
        batches=batches,
        deg_in=deg_in[perm],
        perm=perm,
    )
    return meta, gidx_cores, dstoff_cores, wdeg_cores


def _tile_major(vec, NG, GRP, pad_val):
    """[NPC] -> [GRP, NG]: entry (p, m) = vec[m*GRP + p], padded."""
    out = np.full((NG * GRP,), pad_val, vec.dtype)
    out[: vec.shape[0]] = vec
    return np.ascontiguousarray(out.reshape(NG, GRP).T)


def _build_nc(cfg, meta):
    N, IN, OUT, C = cfg["N"], cfg["IN"], cfg["OUT"], cfg["NCORES"]
    GRP, NBANKS, GC = cfg["GRP"], cfg["NBANKS"], cfg["GCHUNK"]
    NPC, NG, BROWS = meta["NPC"], meta["NG"], meta["BROWS"]
    nidx_tot, nb_tot = meta["nidx_tot"], meta["nb_tot"]
    n_mm = meta["n_mm"]
    block_mms, mm_flags = meta["block_mms"], meta["mm_flags"]
    batches = meta["batches"]
    XK = _ceil_div(IN, 128)
    assert OUT == 128 and GRP == 128 and XK == 2
    last_w = NPC - (NG - 1) * GRP  # valid rows in the last group

    nc = bacc.Bacc(
        "TRN2", target_bir_lowering=False, debug=False, num_devices=C
    )

    # ---- external inputs ----
    xb = [
        nc.dram_tensor(f"xb{q}", [BROWS, IN], BF16, kind="ExternalInput")
        for q in range(NBANKS)
    ]
    wt = [
        nc.dram_tensor(f"wt{j}", [128, OUT], BF16, kind="ExternalInput")
        for j in range(XK)
    ]
    gidx_d = nc.dram_tensor("gidx", [128, nidx_tot // 16], I16, kind="ExternalInput")
    doff_d = nc.dram_tensor("doff", [128, n_mm], F32, kind="ExternalInput")
    wdeg_d = nc.dram_tensor("wdeg", [128, n_mm], F32, kind="ExternalInput")
    degi_d = nc.dram_tensor("degi", [128, NG], F32, kind="ExternalInput")
    bt_d = nc.dram_tensor("bt", [128, OUT], F32, kind="ExternalInput")
    iota_d = nc.dram_tensor("iota", [128, GRP], BF16, kind="ExternalInput")
    ident_d = nc.dram_tensor("ident", [128, 128], BF16, kind="ExternalInput")
    gm_d = nc.dram_tensor("gm", [1, OUT], F32, kind="ExternalInput")
    bb_d = nc.dram_tensor("bb", [1, OUT], F32, kind="ExternalInput")
    onesc_d = nc.dram_tensor("onesc", [128, 1], BF16, kind="ExternalInput")
    ones8_d = nc.dram_tensor("ones8", [8, 1], F32, kind="ExternalInput")
    onest_d = nc.dram_tensor("onest", [128, 1], BF16, kind="ExternalInput")
    onesr_d = nc.dram_tensor("onesr", [1, 128], F32, kind="ExternalInput")

    ypad_d = nc.dram_tensor("ypad", [NG * GRP, OUT], BF16, kind="ExternalOutput")

    with tile.TileContext(nc) as tc:
        with (
            tc.tile_pool(name="const", bufs=1) as cpool,
            tc.tile_pool(name="dram", bufs=1, space="DRAM") as dpool,
            tc.tile_pool(name="hrelu", bufs=1) as hpool,
            tc.tile_pool(name="gath", bufs=5) as gpool,
            tc.tile_pool(name="mpool", bufs=10) as mpool,
            tc.tile_pool(name="capool", bufs=6) as capool,
            tc.tile_pool(name="etmp", bufs=6) as epool,
            tc.tile_pool(name="gtmp", bufs=4) as gpool2,
            tc.tile_pool(name="psagg", bufs=1, space="PSUM") as pagg,
            tc.tile_pool(name="psaux", bufs=1, space="PSUM") as paux,
            tc.tile_pool(name="pstat", bufs=1, space="PSUM") as pspool,
        ):
            # ---- constants / small tiles ----
            iota_t = cpool.tile([128, GRP], BF16)
            ident_t = cpool.tile([128, 128], BF16)
            bt_t = cpool.tile([128, OUT], F32)
            degi_t = cpool.tile([128, NG], F32)
            ndst_t = cpool.tile([128, NG], F32)
            gm_t = cpool.tile([1, OUT], F32)
            bb_t = cpool.tile([1, OUT], F32)
            onesc_t = cpool.tile([128, 1], BF16)
            ones8_t = cpool.tile([8, 1], F32)
            onest_t = cpool.tile([128, 1], BF16)
            onesr_t = cpool.tile([1, 128], F32)
            gidx_t = cpool.tile([128, nidx_tot // 16], I16)
            doff_t = cpool.tile([128, n_mm], F32)
            wsrc_t = cpool.tile([128, n_mm], F32)
            wts = [cpool.tile([128, OUT], BF16, name=f"wt_s{j}") for j in range(XK)]

            # split the big index loads so the first gathers aren't gated on
            # the full-table DMA
            gsplit = min(nidx_tot // 16, 256)
            nc.sync.dma_start(gidx_t[:, :gsplit], gidx_d[:, :gsplit])
            if gsplit < nidx_tot // 16:
                nc.sync.dma_start(gidx_t[:, gsplit:], gidx_d[:, gsplit:])
            dsplit = min(n_mm, 128)
            nc.sync.dma_start(doff_t[:, :dsplit], doff_d[:, :dsplit])
            nc.sync.dma_start(wsrc_t[:, :dsplit], wdeg_d[:, :dsplit])
            if dsplit < n_mm:
                nc.sync.dma_start(doff_t[:, dsplit:], doff_d[:, dsplit:])
                nc.sync.dma_start(wsrc_t[:, dsplit:], wdeg_d[:, dsplit:])
            nc.sync.dma_start(iota_t[:], iota_d[:])
            nc.sync.dma_start(ident_t[:], ident_d[:])
            nc.sync.dma_start(bt_t[:], bt_d[:])
            nc.sync.dma_start(degi_t[:], degi_d[:])
            nc.sync.dma_start(gm_t[:], gm_d[:])
            nc.sync.dma_start(bb_t[:], bb_d[:])
            nc.sync.dma_start(onesc_t[:], onesc_d[:])
            nc.sync.dma_start(ones8_t[:], ones8_d[:])
            nc.sync.dma_start(onest_t[:], onest_d[:])
            nc.sync.dma_start(onesr_t[:], onesr_d[:])
            for j in range(XK):
                nc.sync.dma_start(wts[j][:], wt[j][:])

            # per-edge src norm: w = rsqrt(deg_out[src]) (pad slots carry 1.0);
            # two pieces so the first masks aren't gated on the full tile
            for c0, c1 in ((0, dsplit), (dsplit, n_mm)):
                if c0 >= c1:
                    continue
                nc.vector.reciprocal(wsrc_t[:, c0:c1], wsrc_t[:, c0:c1])
                nc.scalar.activation(
                    wsrc_t[:, c0:c1], wsrc_t[:, c0:c1],
                    mybir.ActivationFunctionType.Sqrt,
                )
            # dst norm: rsqrt(max(deg_in, 1)) tile-major [GRP, NG]
            nc.vector.tensor_scalar(
                ndst_t[:], degi_t[:], 1.0, None, op0=mybir.AluOpType.max
            )
            nc.vector.reciprocal(ndst_t[:], ndst_t[:])
            nc.scalar.activation(
                ndst_t[:], ndst_t[:], mybir.ActivationFunctionType.Sqrt
            )

            stats_in = dpool.tile([1, 2 * OUT], F32)
            _aspace = "Local" if cfg.get("NOCC") else "Shared"
            stats_out = dpool.tile([C, 2 * OUT], F32, addr_space=_aspace)

            hrelu_t = hpool.tile([128, NG, OUT], BF16)

            # ---- PSUM layout (8 banks x 2KB); accumulation-group zeroing is
            # bank-granular, so every concurrently-open chain gets its own
            # bank: 4x agg (GCHUNK groups in flight) + 1x transpose + 2x out
            # (alternating, WAR-tracked) + 1x BN stats (sum+sq as one chain).
            assert GC == 4
            # agg tiles carry a 512B tail used (bitcast bf16) as the per-group
            # transpose target, so each group's finish chain owns its own bank
            ps_agg = [
                pagg.tile([128, 2 * OUT + 128], F32, name=f"ps_agg{i}")
                for i in range(GC)
            ]
            ps_out = [
                paux.tile([128, OUT], F32, name=f"ps_out{i}") for i in range(3)
            ]
            ps_stat = pspool.tile([1, 2, OUT], F32, name="ps_stat")

            # ---- stage D: gather x rows + one-hot matmul segmented sum ----
            def _finish_group(g):
                """Group g's PSUM agg is complete: apply W, relu, BN partials."""
                gi = g % GC
                cagg = capool.tile([128, 2 * OUT], BF16, tag="cagg")
                nc.scalar.activation(
                    cagg[:], ps_agg[gi][:, : 2 * OUT],
                    mybir.ActivationFunctionType.Copy,
                )
                # transpose into this group's own agg bank tail (bitcast bf16)
                # so concurrent groups' finish chains don't share a PSUM bank
                trv = ps_agg[gi][:, 2 * OUT :].bitcast(BF16)
                for h in range(2):
                    nc.tensor.matmul(
                        trv[:, h * OUT : (h + 1) * OUT],
                        cagg[:, h * OUT : (h + 1) * OUT],
                        ident_t[:],
                        is_transpose=True,
                        start=(h == 0),
                        stop=(h == 1),
                    )
                tagg = capool.tile([128, 2, OUT], BF16, tag="tagg")
                nc.scalar.activation(
                    tagg[:, :, :],
                    trv[:].rearrange("p (a f) -> p a f", a=2),
                    mybir.ActivationFunctionType.Copy,
                )
                po = ps_out[g % 3]
                for jj in range(XK):
                    nc.tensor.matmul(
                        po[:],
                        tagg[:, jj, :],
                        wts[jj][:],
                        start=(jj == 0),
                        stop=(jj == XK - 1),
                    )
                # stage E: relu(out * ndst + b), BN partial sums
                tmp = epool.tile([128, OUT], F32, tag="etmp")
                nc.vector.scalar_tensor_tensor(
                    tmp[:],
                    po[:],
                    ndst_t[:, g : g + 1],
                    bt_t[:],
                    op0=mybir.AluOpType.mult,
                    op1=mybir.AluOpType.add,
                )
                nc.scalar.activation(
                    hrelu_t[:, g, :], tmp[:], mybir.ActivationFunctionType.Relu
                )
                ones = onesc_t if g < NG - 1 else onest_t
                nc.tensor.matmul(
                    ps_stat[:, 0, :],
                    ones[:],
                    hrelu_t[:, g, :],
                    start=(g == 0),
                    stop=False,
                )
                sq = epool.tile([128, OUT], BF16, tag="esq")
                nc.scalar.activation(
                    sq[:], hrelu_t[:, g, :], mybir.ActivationFunctionType.Square
                )
                nc.tensor.matmul(
                    ps_stat[:, 1, :],
                    ones[:],
                    sq[:],
                    start=False,
                    stop=(g == NG - 1),
                )

            bmax = max(nb for _, _, nb in batches)
            for bank, t0, nblk in batches:
                Gt = gpool.tile([128, bmax, IN], BF16, tag="G")
                nc.gpsimd.dma_gather(
                    Gt[:, :nblk, :],
                    xb[bank][:],
                    gidx_t[:, t0 * 8 : (t0 + nblk) * 8],
                    nblk * 128,
                    nblk * 128,
                    IN,
                    single_packet=False,
                )
                for j in range(nblk):
                    t = t0 + j
                    for m, g in block_mms[t]:
                      is_start, is_stop = mm_flags[m]
                      gi = g % GC
                      Mt = mpool.tile([128, GRP], BF16, tag="M")
                      nc.vector.tensor_scalar(
                          Mt[:],
                          iota_t[:],
                          doff_t[:, m : m + 1],
                          wsrc_t[:, m : m + 1],
                          op0=mybir.AluOpType.is_equal,
                          op1=mybir.AluOpType.mult,
                      )
                      nc.tensor.matmul(
                          ps_agg[gi][:, : 2 * OUT],
                          Mt[:],
                          Gt[:, j, :],
                          start=is_start,
                          stop=is_stop,
                      )
                      if is_stop:
                          _finish_group(g)

            # ---- stage F: AllReduce BN stats; build affine S/T tiles ----
            S_t = cpool.tile([128, OUT], BF16)
            T_t = cpool.tile([128, OUT], BF16)
            st_sb = cpool.tile([1, 2 * OUT], F32)
            nc.scalar.activation(
                st_sb[:], ps_stat[:].rearrange("p a f -> p (a f)"),
                mybir.ActivationFunctionType.Copy,
            )
            nc.sync.dma_start(stats_in[:], st_sb[:])
            if cfg.get("NOCC"):
                stats_out = stats_in  # single-core debug: sums are the totals
                st8 = st_sb
                ones8v = None
            else:
                # AllGather (no 1.875x reduce multiplier) + tiny local
                # ones-matmul reduction beats AllReduce on latency
                nc.gpsimd.collective_compute(
                    "AllGather",
                    mybir.AluOpType.bypass,
                    replica_groups=[list(range(C))],
                    ins=[stats_in[:]],
                    outs=[stats_out[:]],
                )
                st8 = cpool.tile([C, 2 * OUT], F32)
                nc.sync.dma_start(st8[:], stats_out[:])
                ones8v = ones8_t
            st_rb = cpool.tile([1, 2 * OUT], F32)
            if ones8v is None:
                nc.scalar.activation(
                    st_rb[:], st8[:], mybir.ActivationFunctionType.Copy
                )
            else:
                ps_red = ps_stat[:].rearrange("p a f -> p (a f)")
                nc.tensor.matmul(ps_red, ones8v[:], st8[:], start=True, stop=True)
                nc.scalar.activation(
                    st_rb[:], ps_red, mybir.ActivationFunctionType.Copy
                )

            mu = cpool.tile([1, OUT], F32)
            musq = cpool.tile([1, OUT], F32)
            var = cpool.tile([1, OUT], F32)
            srow = cpool.tile([1, OUT], F32)
            trow = cpool.tile([1, OUT], F32)
            inv_n = 1.0 / float(N)
            nc.scalar.activation(
                mu[:], st_rb[:, 0:OUT], mybir.ActivationFunctionType.Copy, scale=inv_n
            )
            nc.scalar.activation(
                musq[:], mu[:], mybir.ActivationFunctionType.Square
            )
            # var + eps = (E[x^2]*inv_n + eps) - mu^2, then rsqrt via
            # reciprocal+sqrt (ACT Rsqrt is banned for accuracy)
            nc.scalar.activation(
                st_rb[:, OUT : 2 * OUT],
                st_rb[:, OUT : 2 * OUT],
                mybir.ActivationFunctionType.Copy,
                scale=inv_n,
                bias=float(cfg["EPS"]),
            )
            nc.vector.tensor_sub(var[:], st_rb[:, OUT : 2 * OUT], musq[:])
            nc.vector.reciprocal(var[:], var[:])
            nc.scalar.activation(
                var[:], var[:], mybir.ActivationFunctionType.Sqrt
            )
            nc.vector.tensor_mul(srow[:], gm_t[:], var[:])
            nc.vector.tensor_mul(trow[:], mu[:], srow[:])
            nc.vector.tensor_sub(trow[:], bb_t[:], trow[:])

            # reuse the (now idle) out banks for the S/T broadcast matmuls
            nc.tensor.matmul(ps_out[0][:], onesr_t[:], srow[:], start=True, stop=True)
            nc.scalar.activation(
                S_t[:], ps_out[0][:], mybir.ActivationFunctionType.Copy
            )
            nc.tensor.matmul(ps_out[1][:], onesr_t[:], trow[:], start=True, stop=True)
            nc.scalar.activation(
                T_t[:], ps_out[1][:], mybir.ActivationFunctionType.Copy
            )

            # ---- stage G: y = hrelu * S + T (in place, S/T broadcast along
            # the group axis), output DMA chunked to overlap with the DVE ----
            ypad_view = ypad_d[:].rearrange("(g p) f -> p g f", p=128)
            GOUT = 13
            for ci, c0 in enumerate(range(0, NG, GOUT)):
                c1 = min(c0 + GOUT, NG)
                S_b = S_t[:].rearrange("p (a f) -> p a f", a=1).to_broadcast(
                    (128, c1 - c0, OUT)
                )
                T_b = T_t[:].rearrange("p (a f) -> p a f", a=1).to_broadcast(
                    (128, c1 - c0, OUT)
                )
                eng = nc.gpsimd if ci >= 6 else nc.vector
                eng.tensor_mul(
                    hrelu_t[:, c0:c1, :], hrelu_t[:, c0:c1, :], S_b
                )
                eng.tensor_add(
                    hrelu_t[:, c0:c1, :], hrelu_t[:, c0:c1, :], T_b
                )
                nc.sync.dma_start(
                    ypad_view[:, c0:c1, :], hrelu_t[:, c0:c1, :]
                )

    nc.compile()
    return nc


def kernel(x, src, dst, W, b, gamma, beta):
    global LAST_RESULTS
    cfg = CFG
    N, E, IN, OUT, C = cfg["N"], cfg["E"], cfg["IN"], cfg["OUT"], cfg["NCORES"]
    GRP = cfg["GRP"]
    assert x.shape == (N, IN) and W.shape == (IN, OUT)
    assert src.shape == (E,) and dst.shape == (E,)

    meta, gidx_cores, dstoff_cores, wdeg_cores = _preprocess(cfg, src, dst)
    NPC, NG, BROWS = meta["NPC"], meta["NG"], meta["BROWS"]
    XK = _ceil_div(IN, 128)
    last_w = NPC - (NG - 1) * GRP

    nc = _build_nc(cfg, meta)

    import ml_dtypes

    x_bf = np.asarray(x, np.float32).astype(ml_dtypes.bfloat16)
    Wn = np.asarray(W, np.float32)

    iota = np.tile(
        np.arange(GRP, dtype=np.float32)[None, :], (128, 1)
    ).astype(ml_dtypes.bfloat16)
    ident = np.eye(128, dtype=np.float32).astype(ml_dtypes.bfloat16)
    bt = np.tile(np.asarray(b, np.float32)[None, :], (128, 1))
    onesc = np.ones((128, 1), np.float32).astype(ml_dtypes.bfloat16)
    ones8 = np.ones((8, 1), np.float32)
    onest = np.zeros((128, 1), np.float32)
    onest[:last_w] = 1.0
    onest = onest.astype(ml_dtypes.bfloat16)
    onesr = np.ones((1, 128), np.float32)
    gm = np.asarray(gamma, np.float32)[None, :]
    bb = np.asarray(beta, np.float32)[None, :]
    xbanks = [
        np.ascontiguousarray(x_bf[q * BROWS : (q + 1) * BROWS])
        for q in range(cfg["NBANKS"])
    ]
    wtiles = [
        np.ascontiguousarray(Wn[j * 128 : (j + 1) * 128, :]).astype(
            ml_dtypes.bfloat16
        )
        for j in range(XK)
    ]

    in_maps = []
    for k in range(C):
        im = {
            "gidx": gidx_cores[k],
            "doff": dstoff_cores[k],
            "wdeg": wdeg_cores[k],
            "degi": _tile_major(
                meta["deg_in"][k * NPC : (k + 1) * NPC], NG, GRP, np.float32(1.0)
            ),
            "bt": bt,
            "iota": iota,
            "ident": ident,
            "gm": gm,
            "bb": bb,
            "onesc": onesc,
            "ones8": ones8,
            "onest": onest,
            "onesr": onesr,
        }
        for q in range(cfg["NBANKS"]):
            im[f"xb{q}"] = xbanks[q]
        for j in range(XK):
            im[f"wt{j}"] = wtiles[j]
        in_maps.append(im)

    if cfg.get("SIM"):
        from concourse.bass_interp import MultiCoreSim

        sim = MultiCoreSim(nc, num_cores=C)
        for k, core_sim in sim.cores.items():
            for name, val in in_maps[k].items():
                core_sim.tensor(name)[:] = val
        sim.simulate()
        y = np.empty((N, OUT), np.float32)
        perm = meta["perm"]
        for k in range(C):
            y[perm[k * NPC : (k + 1) * NPC]] = np.asarray(
                sim.cores[k].tensor("ypad")[:NPC], dtype=np.float32
            )
        return y

    global LAST_NC, LAST_RUN_S
    LAST_NC = nc
    import time as _time

    _t0 = _time.time()
    res = bass_utils.run_bass_kernel_spmd(
        nc,
        in_maps,
        core_ids=list(range(C)),
        trace=cfg.get("TRACE", False),
    )
    LAST_RUN_S = _time.time() - _t0
    LAST_RESULTS = res

    y = np.empty((N, OUT), np.float32)
    perm = meta["perm"]
    for k in range(C):
        y[perm[k * NPC : (k + 1) * NPC]] = np.asarray(
            res.results[k]["ypad"][:NPC], dtype=np.float32
        )
    return y


# revision 10
# speedup vs baseline: 1.3480x; 1.0272x over previous
"""GCN block (GraphConv + BatchNorm1d + ReLU) on 8 Trainium2 NeuronCores.

Strategy — "gather x, apply W after aggregation":

By linearity, agg[dst] = sum_e norm_src[src_e] * (x[src_e] @ W)
                       = (sum_e norm_src[src_e] * x[src_e]) @ W.
So instead of computing h = x@W per shard and AllGather-ing the h table
across cores (collectives dominated the v1 timeline), every core receives
the FULL x (bf16, row-major) in its own HBM and directly dma_gathers raw x
rows for its edges. No AllGather of features at all, and gathers start at
t=0. x rows are 256 bf16 = 512 B, which also clears the <512 B
small-descriptor DMA penalty that h rows (128 bf16 = 256 B) would pay.

Per core k (owns a permuted 1/8 of the dst nodes):
  1. For each 128-edge block (edges bucketed by (src-bank, dst-group)),
     gather x[src] rows (batched dma_gather, bf16) and segment-sum them with
     one-hot matmuls Mw^T @ G accumulated in PSUM (4 groups in flight, one
     PSUM bank per open accumulation chain). The mask is built on DVE as
     (iota == dstoff) * w_e with w_e = rsqrt(deg_out[src_e]) — norm_src is
     folded into the aggregation.
  2. Per finished dst group: agg [128,256] PSUM -> bf16 SBUF -> PE transpose
     (into a bf16-bitcast tail of the group's own agg bank) -> aggT_j^T @ W_j
     -> out [128, OUT] PSUM; relu(out * rsqrt(clip(deg_in,1)) + b) -> bf16;
     BN partial sums via ones-matmuls into a single PSUM stats chain.
  3. AllGather the [1, 2*OUT] per-core BN sums (cheaper than AllReduce),
     reduce with a ones8 matmul, build the affine S/T tiles, then
     y = hrelu * S + T in-place (S/T broadcast along the group axis, split
     across DVE and GpSimd) with the bf16 output DMA chunk-interleaved.
     The host upcasts y to fp32 (quantization ~0.4% rms << 2e-2 gate).

Host-side work is limited to integer index bookkeeping (edge bucketing,
degree counting, node permutation) and layout transforms (bf16 casts, int16
gather indices). All floating-point math runs on device.

Edges are bucketed by src bank (4 banks of N/4 rows) because dma_gather
indices are int16 (< 32768). Bucket capacities are the exact max over the 8
cores (shared SPMD structure; one NEFF serves every core); dst nodes are
permuted so cores\' same-position buckets have near-identical per-bank edge
counts (lexsort by per-bank degree vector, dealt round-robin), which cuts
the max-over-cores padding to ~3.6%. 128-edge blocks may straddle bucket
boundaries -> one matmul per (block, overlapped group). Pad slots gather
row 0 and carry dst offset 255 -> their one-hot column is all zeros, so
they contribute exactly 0. Banks 0 and 3 get >= 1 slot per group so every
group\'s accumulation chain starts in bank 0 and stops in bank 3, keeping
the BN-stat chain\'s start first / stop last in program order.
"""
import math
import os
import sys

sys.path.insert(0, "/opt/trn_rl_repo")

import numpy as np

import concourse.bacc as bacc
import concourse.bass as bass
import concourse.mybir as mybir
import concourse.tile as tile
from concourse import bass_utils

F32 = mybir.dt.float32
BF16 = mybir.dt.bfloat16
I16 = mybir.dt.int16

CFG = dict(
    N=100000,
    E=1600000,
    IN=256,
    OUT=128,
    NCORES=8,
    GRP=128,          # dst nodes per segment group (= psum partition dim)
    NBANKS=4,         # src banks (bank rows must stay < 32768 for int16 idx)
    GCHUNK=4,         # dst groups in flight (1 PSUM bank per open accum chain)
    BATCH_BLOCKS=40,  # gather batch size in 128-edge blocks
    EPS=1e-5,
    TRACE=False,
)

LAST_RESULTS = None  # set by kernel() for test harness introspection
LAST_NC = None
LAST_RUN_S = None


def _ceil_div(a, b):
    return (a + b - 1) // b


def _wrap16(idx, ncols):
    """int16 idx list -> [128, ncols] tile: idx i at [i%16, i//16], replicated
    8x across the 16-partition groups (one copy per GpSimd Q7 core)."""
    n = idx.shape[0]
    assert n == ncols * 16
    w = np.ascontiguousarray(idx.reshape(ncols, 16).T)
    return np.tile(w, (8, 1))


def _preprocess(cfg, src, dst):
    """Bucket edges by (owner core, src bank, dst group); build per-core
    gather-index / dst-offset / src-degree arrays and the shared block
    structure."""
    N, E = cfg["N"], cfg["E"]
    C, NBANKS, GRP = cfg["NCORES"], cfg["NBANKS"], cfg["GRP"]
    NPC = N // C
    NG = _ceil_div(NPC, GRP)
    assert N % NBANKS == 0
    BROWS = N // NBANKS            # rows per x bank (gather source table)
    assert BROWS < 32768

    src = src.astype(np.int64)
    dst = dst.astype(np.int64)
    deg_out = np.bincount(src, minlength=N).astype(np.float32)
    deg_in = np.bincount(dst, minlength=N).astype(np.float32)
    bank = src // BROWS

    # permute dst nodes into (core, group, slot) positions so that the 8
    # cores' group-i buckets have near-identical per-bank in-edge counts --
    # shrinks the SPMD max-over-cores padding of the shared block structure.
    # Nodes sorted by per-bank count vector are dealt round-robin across
    # cores at each (group, slot).
    nbc = np.zeros((N, NBANKS), np.int64)
    for b in range(NBANKS):
        nbc[:, b] = np.bincount(dst[bank == b], minlength=N)
    node_order = np.lexsort(tuple(-nbc[:, b] for b in range(NBANKS - 1, -1, -1)))
    allpos = np.arange(N)
    deal_order = np.lexsort((allpos // NPC, allpos % GRP, (allpos % NPC) // GRP))
    perm = np.empty(N, np.int64)      # new position -> original node
    perm[deal_order] = node_order
    pos = np.empty(N, np.int64)       # original node -> new position
    pos[perm] = allpos

    pdst = pos[dst]
    owner = pdst // NPC
    grp = (pdst % NPC) // GRP
    key = (owner * NBANKS + bank) * NG + grp
    order = np.argsort(key, kind="stable")
    s_src = src[order]
    s_pdst = pos[dst[order]]
    s_key = key[order]

    counts = np.bincount(key, minlength=C * NBANKS * NG).reshape(C, NBANKS, NG)
    # bucket capacity: exact max over cores (shared SPMD structure); >= 1 in
    # banks 0/3 so every group has a first (bank-0) and last (bank-3) matmul
    P = counts.max(axis=0)  # [NBANKS, NG]
    P[0] = np.maximum(P[0], 1)
    P[NBANKS - 1] = np.maximum(P[NBANKS - 1], 1)

    # stream order: group-chunks outer, banks inner; a group's PSUM slot is
    # live across all banks of its chunk (accumulated with start/stop).
    # Segment (chunk, bank) = that chunk's buckets concatenated, padded to a
    # multiple of 128; 128-edge blocks may straddle bucket (group) boundaries
    # -> one matmul per (block, overlapped group).
    GC = cfg["GCHUNK"]
    chunks = [list(range(c, min(c + GC, NG))) for c in range(0, NG, GC)]
    run_off = np.zeros((NBANKS, NG), np.int64)
    segments = []  # (bank, seg_start_slot, seg_nblocks)
    pos = 0
    for ch in chunks:
        for b in range(NBANKS):
            seg0 = pos
            for g in ch:
                run_off[b, g] = pos
                pos += P[b, g]
            pos = ((pos + 127) // 128) * 128  # segment tail pad
            segments.append((b, seg0, (pos - seg0) // 128))
    nidx_tot = pos
    nb_tot = nidx_tot // 128

    # per block: list of matmuls (mm_col, group); per group: first/last mm id
    block_mms = [[] for _ in range(nb_tot)]
    n_mm = 0
    mm_of_group = {}
    for ch in chunks:
        for b in range(NBANKS):
            for g in ch:
                o0, o1 = run_off[b, g], run_off[b, g] + P[b, g]
                for t in range(o0 // 128, (o1 - 1) // 128 + 1):
                    block_mms[t].append((n_mm, g))
                    mm_of_group.setdefault(g, []).append(n_mm)
                    n_mm += 1
    mm_flags = {}
    for g, mms in mm_of_group.items():
        for m in mms:
            mm_flags[m] = (m == mms[0], m == mms[-1])

    # boundaries of each (k, b, g) bucket in the sorted edge stream
    bkeys = (np.arange(C)[:, None, None] * NBANKS + np.arange(NBANKS)[None, :, None]) * NG + np.arange(NG)[None, None, :]
    starts = np.searchsorted(s_key, bkeys.ravel()).reshape(C, NBANKS, NG)
    ends = np.searchsorted(s_key, bkeys.ravel(), side="right").reshape(C, NBANKS, NG)

    gidx_cores = []
    dstoff_cores = []
    wdeg_cores = []
    for k in range(C):
        gidx = np.zeros(nidx_tot, np.int16)
        doff = np.full((n_mm, 128), 255.0, np.float32)
        wdeg = np.ones((n_mm, 128), np.float32)
        # fill gather indices per bucket (pad slots keep row 0)
        for b in range(NBANKS):
            for g in range(NG):
                s, e = starts[k, b, g], ends[k, b, g]
                cnt = e - s
                if cnt == 0:
                    continue
                p0 = run_off[b, g]
                gidx[p0 : p0 + cnt] = (s_src[s:e] % BROWS).astype(np.int16)
        # fill per-matmul mask columns: rows = this core's real edges of the
        # matmul's group that fall inside the block's 128-slot window
        for ch in chunks:
            for b in range(NBANKS):
                for g in ch:
                    s, e = starts[k, b, g], ends[k, b, g]
                    cnt = e - s
                    o0 = run_off[b, g]
                    bucket_mms = [
                        (m, t)
                        for t in range(o0 // 128, (o0 + P[b, g] - 1) // 128 + 1)
                        for (m, gg) in block_mms[t]
                        if gg == g
                    ]
                    if cnt == 0:
                        continue
                    dvals = ((s_pdst[s:e] % NPC) - g * GRP).astype(np.float32)
                    wvals = deg_out[s_src[s:e]]
                    for m, t in bucket_mms:
                        w0 = t * 128
                        lo = max(o0, w0)
                        hi = min(o0 + cnt, w0 + 128)
                        if lo >= hi:
                            continue
                        rows = np.arange(lo - w0, hi - w0)
                        doff[m, rows] = dvals[lo - o0 : hi - o0]
                        wdeg[m, rows] = wvals[lo - o0 : hi - o0]
        gidx_cores.append(_wrap16(gidx, nidx_tot // 16))
        dstoff_cores.append(np.ascontiguousarray(doff.T))
        wdeg_cores.append(np.ascontiguousarray(wdeg.T))

    # gather batches: split segments longer than BATCH_BLOCKS
    batches = []  # (bank, first_block, n_blocks)
    for b, seg0, nblk in segments:
        t = seg0 // 128
        left = nblk
        while left > 0:
            n = min(left, cfg["BATCH_BLOCKS"])
            batches.append((b, t, n))
            t += n
            left -= n

    meta = dict(
        NPC=NPC,
        NG=NG,
        BROWS=BROWS,
        nidx_tot=nidx_tot,
        nb_tot=nb_tot,
        n_mm=n_mm,
        block_mms=block_mms,
        mm_flags=mm_flags,
        batches=batches,
        deg_in=deg_in[perm],
        perm=perm,
    )
    return meta, gidx_cores, dstoff_cores, wdeg_cores


def _tile_major(vec, NG, GRP, pad_val):
    """[NPC] -> [GRP, NG]: entry (p, m) = vec[m*GRP + p], padded."""
    out = np.full((NG * GRP,), pad_val, vec.dtype)
    out[: vec.shape[0]] = vec
    return np.ascontiguousarray(out.reshape(NG, GRP).T)


def _build_nc(cfg, meta):
    N, IN, OUT, C = cfg["N"], cfg["IN"], cfg["OUT"], cfg["NCORES"]
    GRP, NBANKS, GC = cfg["GRP"], cfg["NBANKS"], cfg["GCHUNK"]
    NPC, NG, BROWS = meta["NPC"], meta["NG"], meta["BROWS"]
    nidx_tot, nb_tot = meta["nidx_tot"], meta["nb_tot"]
    n_mm = meta["n_mm"]
    block_mms, mm_flags = meta["block_mms"], meta["mm_flags"]
    batches = meta["batches"]
    XK = _ceil_div(IN, 128)
    assert OUT == 128 and GRP == 128 and XK == 2
    last_w = NPC - (NG - 1) * GRP  # valid rows in the last group

    nc = bacc.Bacc(
        "TRN2", target_bir_lowering=False, debug=False, num_devices=C
    )

    # ---- external inputs ----
    xb = [
        nc.dram_tensor(f"xb{q}", [BROWS, IN], BF16, kind="ExternalInput")
        for q in range(NBANKS)
    ]
    wt = [
        nc.dram_tensor(f"wt{j}", [128, OUT], BF16, kind="ExternalInput")
        for j in range(XK)
    ]
    gidx_d = nc.dram_tensor("gidx", [128, nidx_tot // 16], I16, kind="ExternalInput")
    doff_d = nc.dram_tensor("doff", [128, n_mm], F32, kind="ExternalInput")
    wdeg_d = nc.dram_tensor("wdeg", [128, n_mm], F32, kind="ExternalInput")
    degi_d = nc.dram_tensor("degi", [128, NG], F32, kind="ExternalInput")
    bt_d = nc.dram_tensor("bt", [128, OUT], F32, kind="ExternalInput")
    iota_d = nc.dram_tensor("iota", [128, GRP], BF16, kind="ExternalInput")
    ident_d = nc.dram_tensor("ident", [128, 128], BF16, kind="ExternalInput")
    gm_d = nc.dram_tensor("gm", [1, OUT], F32, kind="ExternalInput")
    bb_d = nc.dram_tensor("bb", [1, OUT], F32, kind="ExternalInput")
    onesc_d = nc.dram_tensor("onesc", [128, 1], BF16, kind="ExternalInput")
    ones8_d = nc.dram_tensor("ones8", [8, 1], F32, kind="ExternalInput")
    onest_d = nc.dram_tensor("onest", [128, 1], BF16, kind="ExternalInput")
    onesr_d = nc.dram_tensor("onesr", [1, 128], F32, kind="ExternalInput")

    ypad_d = nc.dram_tensor("ypad", [NG * GRP, OUT], BF16, kind="ExternalOutput")

    with tile.TileContext(nc) as tc:
        with (
            tc.tile_pool(name="const", bufs=1) as cpool,
            tc.tile_pool(name="dram", bufs=1, space="DRAM") as dpool,
            tc.tile_pool(name="hrelu", bufs=1) as hpool,
            tc.tile_pool(name="gath", bufs=5) as gpool,
            tc.tile_pool(name="mpool", bufs=10) as mpool,
            tc.tile_pool(name="capool", bufs=6) as capool,
            tc.tile_pool(name="etmp", bufs=6) as epool,
            tc.tile_pool(name="gtmp", bufs=4) as gpool2,
            tc.tile_pool(name="psagg", bufs=1, space="PSUM") as pagg,
            tc.tile_pool(name="psaux", bufs=1, space="PSUM") as paux,
            tc.tile_pool(name="pstat", bufs=1, space="PSUM") as pspool,
        ):
            # ---- constants / small tiles ----
            iota_t = cpool.tile([128, GRP], BF16)
            ident_t = cpool.tile([128, 128], BF16)
            bt_t = cpool.tile([128, OUT], F32)
            degi_t = cpool.tile([128, NG], F32)
            ndst_t = cpool.tile([128, NG], F32)
            gm_t = cpool.tile([1, OUT], F32)
            bb_t = cpool.tile([1, OUT], F32)
            onesc_t = cpool.tile([128, 1], BF16)
            ones8_t = cpool.tile([8, 1], F32)
            onest_t = cpool.tile([128, 1], BF16)
            onesr_t = cpool.tile([1, 128], F32)
            gidx_t = cpool.tile([128, nidx_tot // 16], I16)
            doff_t = cpool.tile([128, n_mm], F32)
            wsrc_t = cpool.tile([128, n_mm], F32)
            wts = [cpool.tile([128, OUT], BF16, name=f"wt_s{j}") for j in range(XK)]

            # split the big index loads so the first gathers aren't gated on
            # the full-table DMA
            gsplit = min(nidx_tot // 16, 256)
            nc.sync.dma_start(gidx_t[:, :gsplit], gidx_d[:, :gsplit])
            if gsplit < nidx_tot // 16:
                nc.sync.dma_start(gidx_t[:, gsplit:], gidx_d[:, gsplit:])
            dsplit = min(n_mm, 128)
            nc.sync.dma_start(doff_t[:, :dsplit], doff_d[:, :dsplit])
            nc.sync.dma_start(wsrc_t[:, :dsplit], wdeg_d[:, :dsplit])
            if dsplit < n_mm:
                nc.sync.dma_start(doff_t[:, dsplit:], doff_d[:, dsplit:])
                nc.sync.dma_start(wsrc_t[:, dsplit:], wdeg_d[:, dsplit:])
            nc.sync.dma_start(iota_t[:], iota_d[:])
            nc.sync.dma_start(ident_t[:], ident_d[:])
            nc.sync.dma_start(bt_t[:], bt_d[:])
            nc.sync.dma_start(degi_t[:], degi_d[:])
            nc.sync.dma_start(gm_t[:], gm_d[:])
            nc.sync.dma_start(bb_t[:], bb_d[:])
            nc.sync.dma_start(onesc_t[:], onesc_d[:])
            nc.sync.dma_start(ones8_t[:], ones8_d[:])
            nc.sync.dma_start(onest_t[:], onest_d[:])
            nc.sync.dma_start(onesr_t[:], onesr_d[:])
            for j in range(XK):
                nc.sync.dma_start(wts[j][:], wt[j][:])

            # per-edge src norm: w = rsqrt(deg_out[src]) (pad slots carry 1.0);
            # two pieces so the first masks aren't gated on the full tile
            for c0, c1 in ((0, dsplit), (dsplit, n_mm)):
                if c0 >= c1:
                    continue
                nc.vector.reciprocal(wsrc_t[:, c0:c1], wsrc_t[:, c0:c1])
                nc.scalar.activation(
                    wsrc_t[:, c0:c1], wsrc_t[:, c0:c1],
                    mybir.ActivationFunctionType.Sqrt,
                )
            # dst norm: rsqrt(max(deg_in, 1)) tile-major [GRP, NG]
            nc.vector.tensor_scalar(
                ndst_t[:], degi_t[:], 1.0, None, op0=mybir.AluOpType.max
            )
            nc.vector.reciprocal(ndst_t[:], ndst_t[:])
            nc.scalar.activation(
                ndst_t[:], ndst_t[:], mybir.ActivationFunctionType.Sqrt
            )

            stats_in = dpool.tile([1, 2 * OUT], F32)
            _aspace = "Local" if cfg.get("NOCC") else "Shared"
            stats_out = dpool.tile([C, 2 * OUT], F32, addr_space=_aspace)

            hrelu_t = hpool.tile([128, NG, OUT], BF16)

            # ---- PSUM layout (8 banks x 2KB); accumulation-group zeroing is
            # bank-granular, so every concurrently-open chain gets its own
            # bank: 4x agg (GCHUNK groups in flight) + 1x transpose + 2x out
            # (alternating, WAR-tracked) + 1x BN stats (sum+sq as one chain).
            assert GC == 4
            # agg tiles carry a 512B tail used (bitcast bf16) as the per-group
            # transpose target, so each group's finish chain owns its own bank
            ps_agg = [
                pagg.tile([128, 2 * OUT + 128], F32, name=f"ps_agg{i}")
                for i in range(GC)
            ]
            ps_out = [
                paux.tile([128, OUT], F32, name=f"ps_out{i}") for i in range(3)
            ]
            ps_stat = pspool.tile([1, 2, OUT], F32, name="ps_stat")

            # ---- stage D: gather x rows + one-hot matmul segmented sum ----
            def _finish_group(g):
                """Group g's PSUM agg is complete: apply W, relu, BN partials."""
                gi = g % GC
                cagg = capool.tile([128, 2 * OUT], BF16, tag="cagg")
                nc.scalar.activation(
                    cagg[:], ps_agg[gi][:, : 2 * OUT],
                    mybir.ActivationFunctionType.Copy,
                )
                # transpose into this group's own agg bank tail (bitcast bf16)
                # so concurrent groups' finish chains don't share a PSUM bank
                trv = ps_agg[gi][:, 2 * OUT :].bitcast(BF16)
                for h in range(2):
                    nc.tensor.matmul(
                        trv[:, h * OUT : (h + 1) * OUT],
                        cagg[:, h * OUT : (h + 1) * OUT],
                        ident_t[:],
                        is_transpose=True,
                        start=(h == 0),
                        stop=(h == 1),
                    )
                tagg = capool.tile([128, 2, OUT], BF16, tag="tagg")
                nc.scalar.activation(
                    tagg[:, :, :],
                    trv[:].rearrange("p (a f) -> p a f", a=2),
                    mybir.ActivationFunctionType.Copy,
                )
                po = ps_out[g % 3]
                for jj in range(XK):
                    nc.tensor.matmul(
                        po[:],
                        tagg[:, jj, :],
                        wts[jj][:],
                        start=(jj == 0),
                        stop=(jj == XK - 1),
                    )
                # stage E: relu(out * ndst + b), BN partial sums
                tmp = epool.tile([128, OUT], F32, tag="etmp")
                nc.vector.scalar_tensor_tensor(
                    tmp[:],
                    po[:],
                    ndst_t[:, g : g + 1],
                    bt_t[:],
                    op0=mybir.AluOpType.mult,
                    op1=mybir.AluOpType.add,
                )
                nc.scalar.activation(
                    hrelu_t[:, g, :], tmp[:], mybir.ActivationFunctionType.Relu
                )
                ones = onesc_t if g < NG - 1 else onest_t
                nc.tensor.matmul(
                    ps_stat[:, 0, :],
                    ones[:],
                    hrelu_t[:, g, :],
                    start=(g == 0),
                    stop=False,
                )
                sq = epool.tile([128, OUT], BF16, tag="esq")
                nc.scalar.activation(
                    sq[:], hrelu_t[:, g, :], mybir.ActivationFunctionType.Square
                )
                nc.tensor.matmul(
                    ps_stat[:, 1, :],
                    ones[:],
                    sq[:],
                    start=False,
                    stop=(g == NG - 1),
                )

            bmax = max(nb for _, _, nb in batches)
            for bank, t0, nblk in batches:
                Gt = gpool.tile([128, bmax, IN], BF16, tag="G")
                nc.gpsimd.dma_gather(
                    Gt[:, :nblk, :],
                    xb[bank][:],
                    gidx_t[:, t0 * 8 : (t0 + nblk) * 8],
                    nblk * 128,
                    nblk * 128,
                    IN,
                    single_packet=False,
                )
                for j in range(nblk):
                    t = t0 + j
                    for m, g in block_mms[t]:
                      is_start, is_stop = mm_flags[m]
                      gi = g % GC
                      Mt = mpool.tile([128, GRP], BF16, tag="M")
                      nc.vector.tensor_scalar(
                          Mt[:],
                          iota_t[:],
                          doff_t[:, m : m + 1],
                          wsrc_t[:, m : m + 1],
                          op0=mybir.AluOpType.is_equal,
                          op1=mybir.AluOpType.mult,
                      )
                      nc.tensor.matmul(
                          ps_agg[gi][:, : 2 * OUT],
                          Mt[:],
                          Gt[:, j, :],
                          start=is_start,
                          stop=is_stop,
                      )
                      if is_stop:
                          _finish_group(g)

            # ---- stage F: AllReduce BN stats; build affine S/T tiles ----
            S_t = cpool.tile([128, OUT], BF16)
            T_t = cpool.tile([128, OUT], BF16)
            st_sb = cpool.tile([1, 2 * OUT], F32)
            nc.scalar.activation(
                st_sb[:], ps_stat[:].rearrange("p a f -> p (a f)"),
                mybir.ActivationFunctionType.Copy,
            )
            nc.sync.dma_start(stats_in[:], st_sb[:])
            if cfg.get("NOCC"):
                stats_out = stats_in  # single-core debug: sums are the totals
                st8 = st_sb
                ones8v = None
            else:
                # AllGather (no 1.875x reduce multiplier) + tiny local
                # ones-matmul reduction beats AllReduce on latency
                nc.gpsimd.collective_compute(
                    "AllGather",
                    mybir.AluOpType.bypass,
                    replica_groups=[list(range(C))],
                    ins=[stats_in[:]],
                    outs=[stats_out[:]],
                )
                st8 = cpool.tile([C, 2 * OUT], F32)
                nc.sync.dma_start(st8[:], stats_out[:])
                ones8v = ones8_t
            if ones8v is None:
                st_rb = cpool.tile([1, 2 * OUT], F32)
                nc.scalar.activation(
                    st_rb[:], st8[:], mybir.ActivationFunctionType.Copy
                )
                red_v = st_rb[:]
            else:
                ps_red = ps_stat[:].rearrange("p a f -> p (a f)")
                nc.tensor.matmul(ps_red, ones8v[:], st8[:], start=True, stop=True)
                red_v = ps_red

            mu = cpool.tile([1, OUT], F32)
            musq = cpool.tile([1, OUT], F32)
            ex2e = cpool.tile([1, OUT], F32)
            var = cpool.tile([1, OUT], F32)
            srow = cpool.tile([1, OUT], F32)
            trow = cpool.tile([1, OUT], F32)
            inv_n = 1.0 / float(N)
            # mu/ex2 straight off the reduction PSUM; fold 1/N and eps in
            nc.scalar.activation(
                mu[:], red_v[:, 0:OUT], mybir.ActivationFunctionType.Copy, scale=inv_n
            )
            nc.scalar.activation(
                ex2e[:],
                red_v[:, OUT : 2 * OUT],
                mybir.ActivationFunctionType.Copy,
                scale=inv_n,
                bias=float(cfg["EPS"]),
            )
            nc.scalar.activation(
                musq[:], mu[:], mybir.ActivationFunctionType.Square
            )
            # var + eps, then rsqrt via reciprocal+sqrt (ACT Rsqrt is banned
            # for accuracy)
            nc.vector.tensor_sub(var[:], ex2e[:], musq[:])
            nc.vector.reciprocal(var[:], var[:])
            nc.scalar.activation(
                var[:], var[:], mybir.ActivationFunctionType.Sqrt
            )
            nc.vector.tensor_mul(srow[:], gm_t[:], var[:])
            nc.vector.tensor_mul(trow[:], mu[:], srow[:])
            nc.vector.tensor_sub(trow[:], bb_t[:], trow[:])

            # reuse the (now idle) out banks for the S/T broadcast matmuls
            nc.tensor.matmul(ps_out[0][:], onesr_t[:], srow[:], start=True, stop=True)
            nc.scalar.activation(
                S_t[:], ps_out[0][:], mybir.ActivationFunctionType.Copy
            )
            nc.tensor.matmul(ps_out[1][:], onesr_t[:], trow[:], start=True, stop=True)
            nc.scalar.activation(
                T_t[:], ps_out[1][:], mybir.ActivationFunctionType.Copy
            )

            # ---- stage G: y = hrelu * S + T (in place, S/T broadcast along
            # the group axis), output DMA chunked to overlap with the DVE ----
            ypad_view = ypad_d[:].rearrange("(g p) f -> p g f", p=128)
            GOUT = 7
            for ci, c0 in enumerate(range(0, NG, GOUT)):
                c1 = min(c0 + GOUT, NG)
                S_b = S_t[:].rearrange("p (a f) -> p a f", a=1).to_broadcast(
                    (128, c1 - c0, OUT)
                )
                T_b = T_t[:].rearrange("p (a f) -> p a f", a=1).to_broadcast(
                    (128, c1 - c0, OUT)
                )
                eng = nc.gpsimd if ci >= 10 else nc.vector
                eng.tensor_mul(
                    hrelu_t[:, c0:c1, :], hrelu_t[:, c0:c1, :], S_b
                )
                eng.tensor_add(
                    hrelu_t[:, c0:c1, :], hrelu_t[:, c0:c1, :], T_b
                )
                nc.sync.dma_start(
                    ypad_view[:, c0:c1, :], hrelu_t[:, c0:c1, :]
                )

    nc.compile()
    return nc


def kernel(x, src, dst, W, b, gamma, beta):
    global LAST_RESULTS
    cfg = CFG
    N, E, IN, OUT, C = cfg["N"], cfg["E"], cfg["IN"], cfg["OUT"], cfg["NCORES"]
    GRP = cfg["GRP"]
    assert x.shape == (N, IN) and W.shape == (IN, OUT)
    assert src.shape == (E,) and dst.shape == (E,)

    meta, gidx_cores, dstoff_cores, wdeg_cores = _preprocess(cfg, src, dst)
    NPC, NG, BROWS = meta["NPC"], meta["NG"], meta["BROWS"]
    XK = _ceil_div(IN, 128)
    last_w = NPC - (NG - 1) * GRP

    nc = _build_nc(cfg, meta)

    import ml_dtypes

    x_bf = np.asarray(x, np.float32).astype(ml_dtypes.bfloat16)
    Wn = np.asarray(W, np.float32)

    iota = np.tile(
        np.arange(GRP, dtype=np.float32)[None, :], (128, 1)
    ).astype(ml_dtypes.bfloat16)
    ident = np.eye(128, dtype=np.float32).astype(ml_dtypes.bfloat16)
    bt = np.tile(np.asarray(b, np.float32)[None, :], (128, 1))
    onesc = np.ones((128, 1), np.float32).astype(ml_dtypes.bfloat16)
    ones8 = np.ones((8, 1), np.float32)
    onest = np.zeros((128, 1), np.float32)
    onest[:last_w] = 1.0
    onest = onest.astype(ml_dtypes.bfloat16)
    onesr = np.ones((1, 128), np.float32)
    gm = np.asarray(gamma, np.float32)[None, :]
    bb = np.asarray(beta, np.float32)[None, :]
    xbanks = [
        np.ascontiguousarray(x_bf[q * BROWS : (q + 1) * BROWS])
        for q in range(cfg["NBANKS"])
    ]
    wtiles = [
        np.ascontiguousarray(Wn[j * 128 : (j + 1) * 128, :]).astype(
            ml_dtypes.bfloat16
        )
        for j in range(XK)
    ]

    in_maps = []
    for k in range(C):
        im = {
            "gidx": gidx_cores[k],
            "doff": dstoff_cores[k],
            "wdeg": wdeg_cores[k],
            "degi": _tile_major(
                meta["deg_in"][k * NPC : (k + 1) * NPC], NG, GRP, np.float32(1.0)
            ),
            "bt": bt,
            "iota": iota,
            "ident": ident,
            "gm": gm,
            "bb": bb,
            "onesc": onesc,
            "ones8": ones8,
            "onest": onest,
            "onesr": onesr,
        }
        for q in range(cfg["NBANKS"]):
            im[f"xb{q}"] = xbanks[q]
        for j in range(XK):
            im[f"wt{j}"] = wtiles[j]
        in_maps.append(im)

    if cfg.get("SIM"):
        from concourse.bass_interp import MultiCoreSim

        sim = MultiCoreSim(nc, num_cores=C)
        for k, core_sim in sim.cores.items():
            for name, val in in_maps[k].items():
                core_sim.tensor(name)[:] = val
        sim.simulate()
        y = np.empty((N, OUT), np.float32)
        perm = meta["perm"]
        for k in range(C):
            y[perm[k * NPC : (k + 1) * NPC]] = np.asarray(
                sim.cores[k].tensor("ypad")[:NPC], dtype=np.float32
            )
        return y

    global LAST_NC, LAST_RUN_S
    LAST_NC = nc
    import time as _time

    _t0 = _time.time()
    res = bass_utils.run_bass_kernel_spmd(
        nc,
        in_maps,
        core_ids=list(range(C)),
        trace=cfg.get("TRACE", False),
    )
    LAST_RUN_S = _time.time() - _t0
    LAST_RESULTS = res

    y = np.empty((N, OUT), np.float32)
    perm = meta["perm"]
    for k in range(C):
        y[perm[k * NPC : (k + 1) * NPC]] = np.asarray(
            res.results[k]["ypad"][:NPC], dtype=np.float32
        )
    return y


# revision 11
# speedup vs baseline: 1.3520x; 1.0030x over previous
"""GCN block (GraphConv + BatchNorm1d + ReLU) on 8 Trainium2 NeuronCores.

Strategy — "gather x, apply W after aggregation":

By linearity, agg[dst] = sum_e norm_src[src_e] * (x[src_e] @ W)
                       = (sum_e norm_src[src_e] * x[src_e]) @ W.
So instead of computing h = x@W per shard and AllGather-ing the h table
across cores (collectives dominated the v1 timeline), every core receives
the FULL x (bf16, row-major) in its own HBM and directly dma_gathers raw x
rows for its edges. No AllGather of features at all, and gathers start at
t=0. x rows are 256 bf16 = 512 B, which also clears the <512 B
small-descriptor DMA penalty that h rows (128 bf16 = 256 B) would pay.

Per core k (owns a permuted 1/8 of the dst nodes):
  1. For each 128-edge block (edges bucketed by (src-bank, dst-group)),
     gather x[src] rows (batched dma_gather, bf16) and segment-sum them with
     one-hot matmuls Mw^T @ G accumulated in PSUM (4 groups in flight, one
     PSUM bank per open accumulation chain). The mask is built on DVE as
     (iota == dstoff) * w_e with w_e = rsqrt(deg_out[src_e]) — norm_src is
     folded into the aggregation.
  2. Per finished dst group: agg [128,256] PSUM -> bf16 SBUF -> PE transpose
     (into a bf16-bitcast tail of the group's own agg bank) -> aggT_j^T @ W_j
     -> out [128, OUT] PSUM; relu(out * rsqrt(clip(deg_in,1)) + b) -> bf16;
     BN partial sums via ones-matmuls into a single PSUM stats chain.
  3. AllGather the [1, 2*OUT] per-core BN sums (cheaper than AllReduce),
     reduce with a ones8 matmul, build the affine S/T tiles, then
     y = hrelu * S + T in-place (S/T broadcast along the group axis, split
     across DVE and GpSimd) with the bf16 output DMA chunk-interleaved.
     The host upcasts y to fp32 (quantization ~0.4% rms << 2e-2 gate).

Host-side work is limited to integer index bookkeeping (edge bucketing,
degree counting, node permutation) and layout transforms (bf16 casts, int16
gather indices). All floating-point math runs on device.

Edges are bucketed by src bank (4 banks of N/4 rows) because dma_gather
indices are int16 (< 32768). Bucket capacities are the exact max over the 8
cores (shared SPMD structure; one NEFF serves every core); dst nodes are
permuted so cores\' same-position buckets have near-identical per-bank edge
counts (lexsort by per-bank degree vector, dealt round-robin), which cuts
the max-over-cores padding to ~3.6%. 128-edge blocks may straddle bucket
boundaries -> one matmul per (block, overlapped group). Pad slots gather
row 0 and carry dst offset 255 -> their one-hot column is all zeros, so
they contribute exactly 0. Banks 0 and 3 get >= 1 slot per group so every
group\'s accumulation chain starts in bank 0 and stops in bank 3, keeping
the BN-stat chain\'s start first / stop last in program order.
"""
import math
import os
import sys

sys.path.insert(0, "/opt/trn_rl_repo")

import numpy as np

import concourse.bacc as bacc
import concourse.bass as bass
import concourse.mybir as mybir
import concourse.tile as tile
from concourse import bass_utils

F32 = mybir.dt.float32
BF16 = mybir.dt.bfloat16
I16 = mybir.dt.int16

CFG = dict(
    N=100000,
    E=1600000,
    IN=256,
    OUT=128,
    NCORES=8,
    GRP=128,          # dst nodes per segment group (= psum partition dim)
    NBANKS=4,         # src banks (bank rows must stay < 32768 for int16 idx)
    GCHUNK=4,         # dst groups in flight (1 PSUM bank per open accum chain)
    BATCH_BLOCKS=12,  # gather batch size in 128-edge blocks
    EPS=1e-5,
    TRACE=False,
)

LAST_RESULTS = None  # set by kernel() for test harness introspection
LAST_NC = None
LAST_RUN_S = None


def _ceil_div(a, b):
    return (a + b - 1) // b


def _wrap16(idx, ncols):
    """int16 idx list -> [128, ncols] tile: idx i at [i%16, i//16], replicated
    8x across the 16-partition groups (one copy per GpSimd Q7 core)."""
    n = idx.shape[0]
    assert n == ncols * 16
    w = np.ascontiguousarray(idx.reshape(ncols, 16).T)
    return np.tile(w, (8, 1))


def _preprocess(cfg, src, dst):
    """Bucket edges by (owner core, src bank, dst group); build per-core
    gather-index / dst-offset / src-degree arrays and the shared block
    structure."""
    N, E = cfg["N"], cfg["E"]
    C, NBANKS, GRP = cfg["NCORES"], cfg["NBANKS"], cfg["GRP"]
    NPC = N // C
    NG = _ceil_div(NPC, GRP)
    assert N % NBANKS == 0
    BROWS = N // NBANKS            # rows per x bank (gather source table)
    assert BROWS < 32768

    src = src.astype(np.int64)
    dst = dst.astype(np.int64)
    deg_out = np.bincount(src, minlength=N).astype(np.float32)
    deg_in = np.bincount(dst, minlength=N).astype(np.float32)
    bank = src // BROWS

    # permute dst nodes into (core, group, slot) positions so that the 8
    # cores' group-i buckets have near-identical per-bank in-edge counts --
    # shrinks the SPMD max-over-cores padding of the shared block structure.
    # Nodes sorted by per-bank count vector are dealt round-robin across
    # cores at each (group, slot).
    nbc = np.zeros((N, NBANKS), np.int64)
    for b in range(NBANKS):
        nbc[:, b] = np.bincount(dst[bank == b], minlength=N)
    node_order = np.lexsort(tuple(-nbc[:, b] for b in range(NBANKS - 1, -1, -1)))
    allpos = np.arange(N)
    deal_order = np.lexsort((allpos // NPC, allpos % GRP, (allpos % NPC) // GRP))
    perm = np.empty(N, np.int64)      # new position -> original node
    perm[deal_order] = node_order
    pos = np.empty(N, np.int64)       # original node -> new position
    pos[perm] = allpos

    pdst = pos[dst]
    owner = pdst // NPC
    grp = (pdst % NPC) // GRP
    key = (owner * NBANKS + bank) * NG + grp
    order = np.argsort(key, kind="stable")
    s_src = src[order]
    s_pdst = pos[dst[order]]
    s_key = key[order]

    counts = np.bincount(key, minlength=C * NBANKS * NG).reshape(C, NBANKS, NG)
    # bucket capacity: exact max over cores (shared SPMD structure); >= 1 in
    # banks 0/3 so every group has a first (bank-0) and last (bank-3) matmul
    P = counts.max(axis=0)  # [NBANKS, NG]
    P[0] = np.maximum(P[0], 1)
    P[NBANKS - 1] = np.maximum(P[NBANKS - 1], 1)

    # stream order: group-chunks outer, banks inner; a group's PSUM slot is
    # live across all banks of its chunk (accumulated with start/stop).
    # Segment (chunk, bank) = that chunk's buckets concatenated, padded to a
    # multiple of 128; 128-edge blocks may straddle bucket (group) boundaries
    # -> one matmul per (block, overlapped group).
    GC = cfg["GCHUNK"]
    chunks = [list(range(c, min(c + GC, NG))) for c in range(0, NG, GC)]
    run_off = np.zeros((NBANKS, NG), np.int64)
    segments = []  # (bank, seg_start_slot, seg_nblocks)
    pos = 0
    for ch in chunks:
        for b in range(NBANKS):
            seg0 = pos
            for g in ch:
                run_off[b, g] = pos
                pos += P[b, g]
            pos = ((pos + 127) // 128) * 128  # segment tail pad
            segments.append((b, seg0, (pos - seg0) // 128))
    nidx_tot = pos
    nb_tot = nidx_tot // 128

    # per block: list of matmuls (mm_col, group); per group: first/last mm id
    block_mms = [[] for _ in range(nb_tot)]
    n_mm = 0
    mm_of_group = {}
    for ch in chunks:
        for b in range(NBANKS):
            for g in ch:
                o0, o1 = run_off[b, g], run_off[b, g] + P[b, g]
                for t in range(o0 // 128, (o1 - 1) // 128 + 1):
                    block_mms[t].append((n_mm, g))
                    mm_of_group.setdefault(g, []).append(n_mm)
                    n_mm += 1
    mm_flags = {}
    for g, mms in mm_of_group.items():
        for m in mms:
            mm_flags[m] = (m == mms[0], m == mms[-1])

    # boundaries of each (k, b, g) bucket in the sorted edge stream
    bkeys = (np.arange(C)[:, None, None] * NBANKS + np.arange(NBANKS)[None, :, None]) * NG + np.arange(NG)[None, None, :]
    starts = np.searchsorted(s_key, bkeys.ravel()).reshape(C, NBANKS, NG)
    ends = np.searchsorted(s_key, bkeys.ravel(), side="right").reshape(C, NBANKS, NG)

    gidx_cores = []
    dstoff_cores = []
    wdeg_cores = []
    for k in range(C):
        gidx = np.zeros(nidx_tot, np.int16)
        doff = np.full((n_mm, 128), 255.0, np.float32)
        wdeg = np.ones((n_mm, 128), np.float32)
        # fill gather indices per bucket (pad slots keep row 0)
        for b in range(NBANKS):
            for g in range(NG):
                s, e = starts[k, b, g], ends[k, b, g]
                cnt = e - s
                if cnt == 0:
                    continue
                p0 = run_off[b, g]
                gidx[p0 : p0 + cnt] = (s_src[s:e] % BROWS).astype(np.int16)
        # fill per-matmul mask columns: rows = this core's real edges of the
        # matmul's group that fall inside the block's 128-slot window
        for ch in chunks:
            for b in range(NBANKS):
                for g in ch:
                    s, e = starts[k, b, g], ends[k, b, g]
                    cnt = e - s
                    o0 = run_off[b, g]
                    bucket_mms = [
                        (m, t)
                        for t in range(o0 // 128, (o0 + P[b, g] - 1) // 128 + 1)
                        for (m, gg) in block_mms[t]
                        if gg == g
                    ]
                    if cnt == 0:
                        continue
                    dvals = ((s_pdst[s:e] % NPC) - g * GRP).astype(np.float32)
                    wvals = deg_out[s_src[s:e]]
                    for m, t in bucket_mms:
                        w0 = t * 128
                        lo = max(o0, w0)
                        hi = min(o0 + cnt, w0 + 128)
                        if lo >= hi:
                            continue
                        rows = np.arange(lo - w0, hi - w0)
                        doff[m, rows] = dvals[lo - o0 : hi - o0]
                        wdeg[m, rows] = wvals[lo - o0 : hi - o0]
        gidx_cores.append(_wrap16(gidx, nidx_tot // 16))
        dstoff_cores.append(np.ascontiguousarray(doff.T))
        wdeg_cores.append(np.ascontiguousarray(wdeg.T))

    # gather batches: split segments longer than BATCH_BLOCKS
    batches = []  # (bank, first_block, n_blocks)
    for b, seg0, nblk in segments:
        t = seg0 // 128
        left = nblk
        while left > 0:
            n = min(left, cfg["BATCH_BLOCKS"])
            batches.append((b, t, n))
            t += n
            left -= n

    meta = dict(
        NPC=NPC,
        NG=NG,
        BROWS=BROWS,
        nidx_tot=nidx_tot,
        nb_tot=nb_tot,
        n_mm=n_mm,
        block_mms=block_mms,
        mm_flags=mm_flags,
        batches=batches,
        deg_in=deg_in[perm],
        perm=perm,
    )
    return meta, gidx_cores, dstoff_cores, wdeg_cores


def _tile_major(vec, NG, GRP, pad_val):
    """[NPC] -> [GRP, NG]: entry (p, m) = vec[m*GRP + p], padded."""
    out = np.full((NG * GRP,), pad_val, vec.dtype)
    out[: vec.shape[0]] = vec
    return np.ascontiguousarray(out.reshape(NG, GRP).T)


def _build_nc(cfg, meta):
    N, IN, OUT, C = cfg["N"], cfg["IN"], cfg["OUT"], cfg["NCORES"]
    GRP, NBANKS, GC = cfg["GRP"], cfg["NBANKS"], cfg["GCHUNK"]
    NPC, NG, BROWS = meta["NPC"], meta["NG"], meta["BROWS"]
    nidx_tot, nb_tot = meta["nidx_tot"], meta["nb_tot"]
    n_mm = meta["n_mm"]
    block_mms, mm_flags = meta["block_mms"], meta["mm_flags"]
    batches = meta["batches"]
    XK = _ceil_div(IN, 128)
    assert OUT == 128 and GRP == 128 and XK == 2
    last_w = NPC - (NG - 1) * GRP  # valid rows in the last group

    nc = bacc.Bacc(
        "TRN2", target_bir_lowering=False, debug=False, num_devices=C
    )

    # ---- external inputs ----
    xb = [
        nc.dram_tensor(f"xb{q}", [BROWS, IN], BF16, kind="ExternalInput")
        for q in range(NBANKS)
    ]
    wt = [
        nc.dram_tensor(f"wt{j}", [128, OUT], BF16, kind="ExternalInput")
        for j in range(XK)
    ]
    gidx_d = nc.dram_tensor("gidx", [128, nidx_tot // 16], I16, kind="ExternalInput")
    doff_d = nc.dram_tensor("doff", [128, n_mm], F32, kind="ExternalInput")
    wdeg_d = nc.dram_tensor("wdeg", [128, n_mm], F32, kind="ExternalInput")
    degi_d = nc.dram_tensor("degi", [128, NG], F32, kind="ExternalInput")
    bt_d = nc.dram_tensor("bt", [128, OUT], F32, kind="ExternalInput")
    iota_d = nc.dram_tensor("iota", [128, GRP], BF16, kind="ExternalInput")
    ident_d = nc.dram_tensor("ident", [128, 128], BF16, kind="ExternalInput")
    gm_d = nc.dram_tensor("gm", [1, OUT], F32, kind="ExternalInput")
    bb_d = nc.dram_tensor("bb", [1, OUT], F32, kind="ExternalInput")
    onesc_d = nc.dram_tensor("onesc", [128, 1], BF16, kind="ExternalInput")
    ones8_d = nc.dram_tensor("ones8", [8, 1], F32, kind="ExternalInput")
    onest_d = nc.dram_tensor("onest", [128, 1], BF16, kind="ExternalInput")
    onesr_d = nc.dram_tensor("onesr", [1, 128], F32, kind="ExternalInput")

    ypad_d = nc.dram_tensor("ypad", [NG * GRP, OUT], BF16, kind="ExternalOutput")

    with tile.TileContext(nc) as tc:
        with (
            tc.tile_pool(name="const", bufs=1) as cpool,
            tc.tile_pool(name="dram", bufs=1, space="DRAM") as dpool,
            tc.tile_pool(name="hrelu", bufs=1) as hpool,
            tc.tile_pool(name="gath", bufs=5) as gpool,
            tc.tile_pool(name="mpool", bufs=10) as mpool,
            tc.tile_pool(name="capool", bufs=6) as capool,
            tc.tile_pool(name="etmp", bufs=6) as epool,
            tc.tile_pool(name="gtmp", bufs=4) as gpool2,
            tc.tile_pool(name="psagg", bufs=1, space="PSUM") as pagg,
            tc.tile_pool(name="psaux", bufs=1, space="PSUM") as paux,
            tc.tile_pool(name="pstat", bufs=1, space="PSUM") as pspool,
        ):
            # ---- constants / small tiles ----
            iota_t = cpool.tile([128, GRP], BF16)
            ident_t = cpool.tile([128, 128], BF16)
            bt_t = cpool.tile([128, OUT], F32)
            degi_t = cpool.tile([128, NG], F32)
            ndst_t = cpool.tile([128, NG], F32)
            gm_t = cpool.tile([1, OUT], F32)
            bb_t = cpool.tile([1, OUT], F32)
            onesc_t = cpool.tile([128, 1], BF16)
            ones8_t = cpool.tile([8, 1], F32)
            onest_t = cpool.tile([128, 1], BF16)
            onesr_t = cpool.tile([1, 128], F32)
            gidx_t = cpool.tile([128, nidx_tot // 16], I16)
            doff_t = cpool.tile([128, n_mm], F32)
            wsrc_t = cpool.tile([128, n_mm], F32)
            wts = [cpool.tile([128, OUT], BF16, name=f"wt_s{j}") for j in range(XK)]

            # split the big index loads so the first gathers aren't gated on
            # the full-table DMA
            gsplit = min(nidx_tot // 16, 256)
            nc.sync.dma_start(gidx_t[:, :gsplit], gidx_d[:, :gsplit])
            if gsplit < nidx_tot // 16:
                nc.sync.dma_start(gidx_t[:, gsplit:], gidx_d[:, gsplit:])
            dsplit = min(n_mm, 128)
            nc.sync.dma_start(doff_t[:, :dsplit], doff_d[:, :dsplit])
            nc.sync.dma_start(wsrc_t[:, :dsplit], wdeg_d[:, :dsplit])
            if dsplit < n_mm:
                nc.sync.dma_start(doff_t[:, dsplit:], doff_d[:, dsplit:])
                nc.sync.dma_start(wsrc_t[:, dsplit:], wdeg_d[:, dsplit:])
            nc.sync.dma_start(iota_t[:], iota_d[:])
            nc.sync.dma_start(ident_t[:], ident_d[:])
            nc.sync.dma_start(bt_t[:], bt_d[:])
            nc.sync.dma_start(degi_t[:], degi_d[:])
            nc.sync.dma_start(gm_t[:], gm_d[:])
            nc.sync.dma_start(bb_t[:], bb_d[:])
            nc.sync.dma_start(onesc_t[:], onesc_d[:])
            nc.sync.dma_start(ones8_t[:], ones8_d[:])
            nc.sync.dma_start(onest_t[:], onest_d[:])
            nc.sync.dma_start(onesr_t[:], onesr_d[:])
            for j in range(XK):
                nc.sync.dma_start(wts[j][:], wt[j][:])

            # per-edge src norm: w = rsqrt(deg_out[src]) (pad slots carry 1.0);
            # two pieces so the first masks aren't gated on the full tile
            for c0, c1 in ((0, dsplit), (dsplit, n_mm)):
                if c0 >= c1:
                    continue
                nc.vector.reciprocal(wsrc_t[:, c0:c1], wsrc_t[:, c0:c1])
                nc.scalar.activation(
                    wsrc_t[:, c0:c1], wsrc_t[:, c0:c1],
                    mybir.ActivationFunctionType.Sqrt,
                )
            # dst norm: rsqrt(max(deg_in, 1)) tile-major [GRP, NG]
            nc.vector.tensor_scalar(
                ndst_t[:], degi_t[:], 1.0, None, op0=mybir.AluOpType.max
            )
            nc.vector.reciprocal(ndst_t[:], ndst_t[:])
            nc.scalar.activation(
                ndst_t[:], ndst_t[:], mybir.ActivationFunctionType.Sqrt
            )

            stats_in = dpool.tile([1, 2 * OUT], F32)
            _aspace = "Local" if cfg.get("NOCC") else "Shared"
            stats_out = dpool.tile([C, 2 * OUT], F32, addr_space=_aspace)

            hrelu_t = hpool.tile([128, NG, OUT], BF16)

            # ---- PSUM layout (8 banks x 2KB); accumulation-group zeroing is
            # bank-granular, so every concurrently-open chain gets its own
            # bank: 4x agg (GCHUNK groups in flight) + 1x transpose + 2x out
            # (alternating, WAR-tracked) + 1x BN stats (sum+sq as one chain).
            assert GC in (4, 5)
            # agg tiles carry a 512B tail used (bitcast bf16) as the per-group
            # transpose target, so each group's finish chain owns its own bank
            ps_agg = [
                pagg.tile([128, 2 * OUT + 128], F32, name=f"ps_agg{i}")
                for i in range(GC)
            ]
            n_out = 8 - GC - 1
            ps_out = [
                paux.tile([128, OUT], F32, name=f"ps_out{i}") for i in range(n_out)
            ]
            ps_stat = pspool.tile([1, 2, OUT], F32, name="ps_stat")

            # ---- stage D: gather x rows + one-hot matmul segmented sum ----
            def _finish_group(g):
                """Group g's PSUM agg is complete: apply W, relu, BN partials."""
                gi = g % GC
                cagg = capool.tile([128, 2 * OUT], BF16, tag="cagg")
                nc.scalar.activation(
                    cagg[:], ps_agg[gi][:, : 2 * OUT],
                    mybir.ActivationFunctionType.Copy,
                )
                # transpose into this group's own agg bank tail (bitcast bf16)
                # so concurrent groups' finish chains don't share a PSUM bank
                trv = ps_agg[gi][:, 2 * OUT :].bitcast(BF16)
                for h in range(2):
                    nc.tensor.matmul(
                        trv[:, h * OUT : (h + 1) * OUT],
                        cagg[:, h * OUT : (h + 1) * OUT],
                        ident_t[:],
                        is_transpose=True,
                        start=(h == 0),
                        stop=(h == 1),
                    )
                tagg = capool.tile([128, 2, OUT], BF16, tag="tagg")
                nc.scalar.activation(
                    tagg[:, :, :],
                    trv[:].rearrange("p (a f) -> p a f", a=2),
                    mybir.ActivationFunctionType.Copy,
                )
                po = ps_out[g % len(ps_out)]
                for jj in range(XK):
                    nc.tensor.matmul(
                        po[:],
                        tagg[:, jj, :],
                        wts[jj][:],
                        start=(jj == 0),
                        stop=(jj == XK - 1),
                    )
                # stage E: relu(out * ndst + b), BN partial sums
                tmp = epool.tile([128, OUT], F32, tag="etmp")
                nc.vector.scalar_tensor_tensor(
                    tmp[:],
                    po[:],
                    ndst_t[:, g : g + 1],
                    bt_t[:],
                    op0=mybir.AluOpType.mult,
                    op1=mybir.AluOpType.add,
                )
                nc.scalar.activation(
                    hrelu_t[:, g, :], tmp[:], mybir.ActivationFunctionType.Relu
                )
                ones = onesc_t if g < NG - 1 else onest_t
                nc.tensor.matmul(
                    ps_stat[:, 0, :],
                    ones[:],
                    hrelu_t[:, g, :],
                    start=(g == 0),
                    stop=False,
                )
                sq = epool.tile([128, OUT], BF16, tag="esq")
                nc.scalar.activation(
                    sq[:], hrelu_t[:, g, :], mybir.ActivationFunctionType.Square
                )
                nc.tensor.matmul(
                    ps_stat[:, 1, :],
                    ones[:],
                    sq[:],
                    start=False,
                    stop=(g == NG - 1),
                )

            bmax = max(nb for _, _, nb in batches)
            for bank, t0, nblk in batches:
                Gt = gpool.tile([128, bmax, IN], BF16, tag="G")
                nc.gpsimd.dma_gather(
                    Gt[:, :nblk, :],
                    xb[bank][:],
                    gidx_t[:, t0 * 8 : (t0 + nblk) * 8],
                    nblk * 128,
                    nblk * 128,
                    IN,
                    single_packet=False,
                )
                for j in range(nblk):
                    t = t0 + j
                    for m, g in block_mms[t]:
                      is_start, is_stop = mm_flags[m]
                      gi = g % GC
                      Mt = mpool.tile([128, GRP], BF16, tag="M")
                      nc.vector.tensor_scalar(
                          Mt[:],
                          iota_t[:],
                          doff_t[:, m : m + 1],
                          wsrc_t[:, m : m + 1],
                          op0=mybir.AluOpType.is_equal,
                          op1=mybir.AluOpType.mult,
                      )
                      nc.tensor.matmul(
                          ps_agg[gi][:, : 2 * OUT],
                          Mt[:],
                          Gt[:, j, :],
                          start=is_start,
                          stop=is_stop,
                      )
                      if is_stop:
                          _finish_group(g)

            # ---- stage F: AllReduce BN stats; build affine S/T tiles ----
            S_t = cpool.tile([128, OUT], BF16)
            T_t = cpool.tile([128, OUT], BF16)
            st_sb = cpool.tile([1, 2 * OUT], F32)
            nc.scalar.activation(
                st_sb[:], ps_stat[:].rearrange("p a f -> p (a f)"),
                mybir.ActivationFunctionType.Copy,
            )
            nc.sync.dma_start(stats_in[:], st_sb[:])
            if cfg.get("NOCC"):
                stats_out = stats_in  # single-core debug: sums are the totals
                st8 = st_sb
                ones8v = None
            else:
                # AllGather (no 1.875x reduce multiplier) + tiny local
                # ones-matmul reduction beats AllReduce on latency
                nc.gpsimd.collective_compute(
                    "AllGather",
                    mybir.AluOpType.bypass,
                    replica_groups=[list(range(C))],
                    ins=[stats_in[:]],
                    outs=[stats_out[:]],
                )
                st8 = cpool.tile([C, 2 * OUT], F32)
                nc.sync.dma_start(st8[:], stats_out[:])
                ones8v = ones8_t
            if ones8v is None:
                st_rb = cpool.tile([1, 2 * OUT], F32)
                nc.scalar.activation(
                    st_rb[:], st8[:], mybir.ActivationFunctionType.Copy
                )
                red_v = st_rb[:]
            else:
                ps_red = ps_stat[:].rearrange("p a f -> p (a f)")
                nc.tensor.matmul(ps_red, ones8v[:], st8[:], start=True, stop=True)
                red_v = ps_red

            mu = cpool.tile([1, OUT], F32)
            musq = cpool.tile([1, OUT], F32)
            ex2e = cpool.tile([1, OUT], F32)
            var = cpool.tile([1, OUT], F32)
            srow = cpool.tile([1, OUT], F32)
            trow = cpool.tile([1, OUT], F32)
            inv_n = 1.0 / float(N)
            # mu/ex2 straight off the reduction PSUM; fold 1/N and eps in
            nc.scalar.activation(
                mu[:], red_v[:, 0:OUT], mybir.ActivationFunctionType.Copy, scale=inv_n
            )
            nc.scalar.activation(
                ex2e[:],
                red_v[:, OUT : 2 * OUT],
                mybir.ActivationFunctionType.Copy,
                scale=inv_n,
                bias=float(cfg["EPS"]),
            )
            nc.scalar.activation(
                musq[:], mu[:], mybir.ActivationFunctionType.Square
            )
            # var + eps, then rsqrt via reciprocal+sqrt (ACT Rsqrt is banned
            # for accuracy)
            nc.vector.tensor_sub(var[:], ex2e[:], musq[:])
            nc.vector.reciprocal(var[:], var[:])
            nc.scalar.activation(
                var[:], var[:], mybir.ActivationFunctionType.Sqrt
            )
            nc.vector.tensor_mul(srow[:], gm_t[:], var[:])
            nc.vector.tensor_mul(trow[:], mu[:], srow[:])
            nc.vector.tensor_sub(trow[:], bb_t[:], trow[:])

            # reuse the (now idle) out banks for the S/T broadcast matmuls
            nc.tensor.matmul(ps_out[0][:], onesr_t[:], srow[:], start=True, stop=True)
            nc.scalar.activation(
                S_t[:], ps_out[0][:], mybir.ActivationFunctionType.Copy
            )
            nc.tensor.matmul(ps_out[1][:], onesr_t[:], trow[:], start=True, stop=True)
            nc.scalar.activation(
                T_t[:], ps_out[1][:], mybir.ActivationFunctionType.Copy
            )

            # ---- stage G: y = hrelu * S + T (in place, S/T broadcast along
            # the group axis), output DMA chunked to overlap with the DVE ----
            ypad_view = ypad_d[:].rearrange("(g p) f -> p g f", p=128)
            GOUT = 7
            for ci, c0 in enumerate(range(0, NG, GOUT)):
                c1 = min(c0 + GOUT, NG)
                S_b = S_t[:].rearrange("p (a f) -> p a f", a=1).to_broadcast(
                    (128, c1 - c0, OUT)
                )
                T_b = T_t[:].rearrange("p (a f) -> p a f", a=1).to_broadcast(
                    (128, c1 - c0, OUT)
                )
                eng = nc.gpsimd if ci >= 10 else nc.vector
                eng.tensor_mul(
                    hrelu_t[:, c0:c1, :], hrelu_t[:, c0:c1, :], S_b
                )
                eng.tensor_add(
                    hrelu_t[:, c0:c1, :], hrelu_t[:, c0:c1, :], T_b
                )
                nc.sync.dma_start(
                    ypad_view[:, c0:c1, :], hrelu_t[:, c0:c1, :]
                )

    nc.compile()
    return nc


def kernel(x, src, dst, W, b, gamma, beta):
    global LAST_RESULTS
    cfg = CFG
    N, E, IN, OUT, C = cfg["N"], cfg["E"], cfg["IN"], cfg["OUT"], cfg["NCORES"]
    GRP = cfg["GRP"]
    assert x.shape == (N, IN) and W.shape == (IN, OUT)
    assert src.shape == (E,) and dst.shape == (E,)

    meta, gidx_cores, dstoff_cores, wdeg_cores = _preprocess(cfg, src, dst)
    NPC, NG, BROWS = meta["NPC"], meta["NG"], meta["BROWS"]
    XK = _ceil_div(IN, 128)
    last_w = NPC - (NG - 1) * GRP

    nc = _build_nc(cfg, meta)

    import ml_dtypes

    x_bf = np.asarray(x, np.float32).astype(ml_dtypes.bfloat16)
    Wn = np.asarray(W, np.float32)

    iota = np.tile(
        np.arange(GRP, dtype=np.float32)[None, :], (128, 1)
    ).astype(ml_dtypes.bfloat16)
    ident = np.eye(128, dtype=np.float32).astype(ml_dtypes.bfloat16)
    bt = np.tile(np.asarray(b, np.float32)[None, :], (128, 1))
    onesc = np.ones((128, 1), np.float32).astype(ml_dtypes.bfloat16)
    ones8 = np.ones((8, 1), np.float32)
    onest = np.zeros((128, 1), np.float32)
    onest[:last_w] = 1.0
    onest = onest.astype(ml_dtypes.bfloat16)
    onesr = np.ones((1, 128), np.float32)
    gm = np.asarray(gamma, np.float32)[None, :]
    bb = np.asarray(beta, np.float32)[None, :]
    xbanks = [
        np.ascontiguousarray(x_bf[q * BROWS : (q + 1) * BROWS])
        for q in range(cfg["NBANKS"])
    ]
    wtiles = [
        np.ascontiguousarray(Wn[j * 128 : (j + 1) * 128, :]).astype(
            ml_dtypes.bfloat16
        )
        for j in range(XK)
    ]

    in_maps = []
    for k in range(C):
        im = {
            "gidx": gidx_cores[k],
            "doff": dstoff_cores[k],
            "wdeg": wdeg_cores[k],
            "degi": _tile_major(
                meta["deg_in"][k * NPC : (k + 1) * NPC], NG, GRP, np.float32(1.0)
            ),
            "bt": bt,
            "iota": iota,
            "ident": ident,
            "gm": gm,
            "bb": bb,
            "onesc": onesc,
            "ones8": ones8,
            "onest": onest,
            "onesr": onesr,
        }
        for q in range(cfg["NBANKS"]):
            im[f"xb{q}"] = xbanks[q]
        for j in range(XK):
            im[f"wt{j}"] = wtiles[j]
        in_maps.append(im)

    if cfg.get("SIM"):
        from concourse.bass_interp import MultiCoreSim

        sim = MultiCoreSim(nc, num_cores=C)
        for k, core_sim in sim.cores.items():
            for name, val in in_maps[k].items():
                core_sim.tensor(name)[:] = val
        sim.simulate()
        y = np.empty((N, OUT), np.float32)
        perm = meta["perm"]
        for k in range(C):
            y[perm[k * NPC : (k + 1) * NPC]] = np.asarray(
                sim.cores[k].tensor("ypad")[:NPC], dtype=np.float32
            )
        return y

    global LAST_NC, LAST_RUN_S
    LAST_NC = nc
    import time as _time

    _t0 = _time.time()
    res = bass_utils.run_bass_kernel_spmd(
        nc,
        in_maps,
        core_ids=list(range(C)),
        trace=cfg.get("TRACE", False),
    )
    LAST_RUN_S = _time.time() - _t0
    LAST_RESULTS = res

    y = np.empty((N, OUT), np.float32)
    perm = meta["perm"]
    for k in range(C):
        y[perm[k * NPC : (k + 1) * NPC]] = np.asarray(
            res.results[k]["ypad"][:NPC], dtype=np.float32
        )
    return y


# revision 12
# speedup vs baseline: 1.3663x; 1.0106x over previous
"""GCN block (GraphConv + BatchNorm1d + ReLU) on 8 Trainium2 NeuronCores.

Strategy — "gather x, apply W after aggregation":

By linearity, agg[dst] = sum_e norm_src[src_e] * (x[src_e] @ W)
                       = (sum_e norm_src[src_e] * x[src_e]) @ W.
So instead of computing h = x@W per shard and AllGather-ing the h table
across cores (collectives dominated the v1 timeline), every core receives
the FULL x (bf16, row-major) in its own HBM and directly dma_gathers raw x
rows for its edges. No AllGather of features at all, and gathers start at
t=0. x rows are 256 bf16 = 512 B, which also clears the <512 B
small-descriptor DMA penalty that h rows (128 bf16 = 256 B) would pay.

Per core k (owns a permuted 1/8 of the dst nodes):
  1. For each 128-edge block (edges bucketed by (src-bank, dst-group)),
     gather x[src] rows (batched dma_gather, bf16) and segment-sum them with
     one-hot matmuls Mw^T @ G accumulated in PSUM (4 groups in flight, one
     PSUM bank per open accumulation chain). The mask is built on DVE as
     (iota == dstoff) * w_e with w_e = rsqrt(deg_out[src_e]) — norm_src is
     folded into the aggregation.
  2. Per finished dst group: agg [128,256] PSUM -> bf16 SBUF -> PE transpose
     (into a bf16-bitcast tail of the group's own agg bank) -> aggT_j^T @ W_j
     -> out [128, OUT] PSUM; relu(out * rsqrt(clip(deg_in,1)) + b) -> bf16;
     BN partial sums via ones-matmuls into a single PSUM stats chain.
  3. AllGather the [1, 2*OUT] per-core BN sums (cheaper than AllReduce),
     reduce with a ones8 matmul, build the affine S/T tiles, then
     y = hrelu * S + T in-place (S/T broadcast along the group axis, split
     across DVE and GpSimd) with the bf16 output DMA chunk-interleaved.
     The host upcasts y to fp32 (quantization ~0.4% rms << 2e-2 gate).

Host-side work is limited to integer index bookkeeping (edge bucketing,
degree counting, node permutation) and layout transforms (bf16 casts, int16
gather indices). All floating-point math runs on device.

Edges are bucketed by src bank (4 banks of N/4 rows) because dma_gather
indices are int16 (< 32768). Bucket capacities are the exact max over the 8
cores (shared SPMD structure; one NEFF serves every core); dst nodes are
permuted so cores\' same-position buckets have near-identical per-bank edge
counts (lexsort by per-bank degree vector, dealt round-robin), which cuts
the max-over-cores padding to ~3.6%. 128-edge blocks may straddle bucket
boundaries -> one matmul per (block, overlapped group). Pad slots gather
row 0 and carry dst offset 255 -> their one-hot column is all zeros, so
they contribute exactly 0. Banks 0 and 3 get >= 1 slot per group so every
group\'s accumulation chain starts in bank 0 and stops in bank 3, keeping
the BN-stat chain\'s start first / stop last in program order.
"""
import math
import os
import sys

sys.path.insert(0, "/opt/trn_rl_repo")

import numpy as np

import concourse.bacc as bacc
import concourse.bass as bass
import concourse.mybir as mybir
import concourse.tile as tile
from concourse import bass_utils

F32 = mybir.dt.float32
BF16 = mybir.dt.bfloat16
I16 = mybir.dt.int16

CFG = dict(
    N=100000,
    E=1600000,
    IN=256,
    OUT=128,
    NCORES=8,
    GRP=128,          # dst nodes per segment group (= psum partition dim)
    NBANKS=4,         # src banks (bank rows must stay < 32768 for int16 idx)
    GCHUNK=4,         # dst groups in flight (1 PSUM bank per open accum chain)
    BATCH_BLOCKS=12,  # gather batch size in 128-edge blocks
    EPS=1e-5,
    TRACE=False,
)

LAST_RESULTS = None  # set by kernel() for test harness introspection
LAST_NC = None
LAST_RUN_S = None


def _ceil_div(a, b):
    return (a + b - 1) // b


def _wrap16(idx, ncols):
    """int16 idx list -> [128, ncols] tile: idx i at [i%16, i//16], replicated
    8x across the 16-partition groups (one copy per GpSimd Q7 core)."""
    n = idx.shape[0]
    assert n == ncols * 16
    w = np.ascontiguousarray(idx.reshape(ncols, 16).T)
    return np.tile(w, (8, 1))


def _preprocess(cfg, src, dst):
    """Bucket edges by (owner core, src bank, dst group); build per-core
    gather-index / dst-offset / src-degree arrays and the shared block
    structure."""
    N, E = cfg["N"], cfg["E"]
    C, NBANKS, GRP = cfg["NCORES"], cfg["NBANKS"], cfg["GRP"]
    NPC = N // C
    NG = _ceil_div(NPC, GRP)
    assert N % NBANKS == 0
    BROWS = N // NBANKS            # rows per x bank (gather source table)
    assert BROWS < 32768

    src = src.astype(np.int64)
    dst = dst.astype(np.int64)
    deg_out = np.bincount(src, minlength=N).astype(np.float32)
    deg_in = np.bincount(dst, minlength=N).astype(np.float32)
    bank = src // BROWS

    # permute dst nodes into (core, group, slot) positions so that the 8
    # cores' group-i buckets have near-identical per-bank in-edge counts --
    # shrinks the SPMD max-over-cores padding of the shared block structure.
    # Nodes sorted by per-bank count vector are dealt round-robin across
    # cores at each (group, slot).
    nbc = np.zeros((N, NBANKS), np.int64)
    for b in range(NBANKS):
        nbc[:, b] = np.bincount(dst[bank == b], minlength=N)
    node_order = np.lexsort(tuple(-nbc[:, b] for b in range(NBANKS - 1, -1, -1)))
    allpos = np.arange(N)
    deal_order = np.lexsort((allpos // NPC, allpos % GRP, (allpos % NPC) // GRP))
    perm = np.empty(N, np.int64)      # new position -> original node
    perm[deal_order] = node_order
    pos = np.empty(N, np.int64)       # original node -> new position
    pos[perm] = allpos

    pdst = pos[dst]
    owner = pdst // NPC
    grp = (pdst % NPC) // GRP
    key = (owner * NBANKS + bank) * NG + grp
    order = np.argsort(key, kind="stable")
    s_src = src[order]
    s_pdst = pos[dst[order]]
    s_key = key[order]

    counts = np.bincount(key, minlength=C * NBANKS * NG).reshape(C, NBANKS, NG)
    # bucket capacity: exact max over cores (shared SPMD structure); >= 1 in
    # banks 0/3 so every group has a first (bank-0) and last (bank-3) matmul
    P = counts.max(axis=0)  # [NBANKS, NG]
    P[0] = np.maximum(P[0], 1)
    P[NBANKS - 1] = np.maximum(P[NBANKS - 1], 1)

    # stream order: group-chunks outer, banks inner; a group's PSUM slot is
    # live across all banks of its chunk (accumulated with start/stop).
    # Segment (chunk, bank) = that chunk's buckets concatenated, padded to a
    # multiple of 128; 128-edge blocks may straddle bucket (group) boundaries
    # -> one matmul per (block, overlapped group).
    GC = cfg["GCHUNK"]
    chunks = [list(range(c, min(c + GC, NG))) for c in range(0, NG, GC)]
    run_off = np.zeros((NBANKS, NG), np.int64)
    segments = []  # (bank, seg_start_slot, seg_nblocks)
    pos = 0
    for ch in chunks:
        for b in range(NBANKS):
            seg0 = pos
            for g in ch:
                run_off[b, g] = pos
                pos += P[b, g]
            pos = ((pos + 127) // 128) * 128  # segment tail pad
            segments.append((b, seg0, (pos - seg0) // 128))
    nidx_tot = pos
    nb_tot = nidx_tot // 128

    # per block: list of matmuls (mm_col, group); per group: first/last mm id
    block_mms = [[] for _ in range(nb_tot)]
    n_mm = 0
    mm_of_group = {}
    for ch in chunks:
        for b in range(NBANKS):
            for g in ch:
                o0, o1 = run_off[b, g], run_off[b, g] + P[b, g]
                for t in range(o0 // 128, (o1 - 1) // 128 + 1):
                    block_mms[t].append((n_mm, g))
                    mm_of_group.setdefault(g, []).append(n_mm)
                    n_mm += 1
    mm_flags = {}
    for g, mms in mm_of_group.items():
        for m in mms:
            mm_flags[m] = (m == mms[0], m == mms[-1])

    # boundaries of each (k, b, g) bucket in the sorted edge stream
    bkeys = (np.arange(C)[:, None, None] * NBANKS + np.arange(NBANKS)[None, :, None]) * NG + np.arange(NG)[None, None, :]
    starts = np.searchsorted(s_key, bkeys.ravel()).reshape(C, NBANKS, NG)
    ends = np.searchsorted(s_key, bkeys.ravel(), side="right").reshape(C, NBANKS, NG)

    gidx_cores = []
    dstoff_cores = []
    wdeg_cores = []
    for k in range(C):
        gidx = np.zeros(nidx_tot, np.int16)
        doff = np.full((n_mm, 128), 255.0, np.float32)
        wdeg = np.ones((n_mm, 128), np.float32)
        # fill gather indices per bucket (pad slots keep row 0)
        for b in range(NBANKS):
            for g in range(NG):
                s, e = starts[k, b, g], ends[k, b, g]
                cnt = e - s
                if cnt == 0:
                    continue
                p0 = run_off[b, g]
                gidx[p0 : p0 + cnt] = (s_src[s:e] % BROWS).astype(np.int16)
        # fill per-matmul mask columns: rows = this core's real edges of the
        # matmul's group that fall inside the block's 128-slot window
        for ch in chunks:
            for b in range(NBANKS):
                for g in ch:
                    s, e = starts[k, b, g], ends[k, b, g]
                    cnt = e - s
                    o0 = run_off[b, g]
                    bucket_mms = [
                        (m, t)
                        for t in range(o0 // 128, (o0 + P[b, g] - 1) // 128 + 1)
                        for (m, gg) in block_mms[t]
                        if gg == g
                    ]
                    if cnt == 0:
                        continue
                    dvals = ((s_pdst[s:e] % NPC) - g * GRP).astype(np.float32)
                    wvals = deg_out[s_src[s:e]]
                    for m, t in bucket_mms:
                        w0 = t * 128
                        lo = max(o0, w0)
                        hi = min(o0 + cnt, w0 + 128)
                        if lo >= hi:
                            continue
                        rows = np.arange(lo - w0, hi - w0)
                        doff[m, rows] = dvals[lo - o0 : hi - o0]
                        wdeg[m, rows] = wvals[lo - o0 : hi - o0]
        gidx_cores.append(_wrap16(gidx, nidx_tot // 16))
        dstoff_cores.append(np.ascontiguousarray(doff.T))
        wdeg_cores.append(np.ascontiguousarray(wdeg.T))

    # gather batches: split segments longer than BATCH_BLOCKS
    batches = []  # (bank, first_block, n_blocks)
    for b, seg0, nblk in segments:
        t = seg0 // 128
        left = nblk
        while left > 0:
            n = min(left, cfg["BATCH_BLOCKS"])
            batches.append((b, t, n))
            t += n
            left -= n

    meta = dict(
        NPC=NPC,
        NG=NG,
        BROWS=BROWS,
        nidx_tot=nidx_tot,
        nb_tot=nb_tot,
        n_mm=n_mm,
        block_mms=block_mms,
        mm_flags=mm_flags,
        batches=batches,
        deg_in=deg_in[perm],
        perm=perm,
    )
    return meta, gidx_cores, dstoff_cores, wdeg_cores


def _tile_major(vec, NG, GRP, pad_val):
    """[NPC] -> [GRP, NG]: entry (p, m) = vec[m*GRP + p], padded."""
    out = np.full((NG * GRP,), pad_val, vec.dtype)
    out[: vec.shape[0]] = vec
    return np.ascontiguousarray(out.reshape(NG, GRP).T)


def _build_nc(cfg, meta):
    N, IN, OUT, C = cfg["N"], cfg["IN"], cfg["OUT"], cfg["NCORES"]
    GRP, NBANKS, GC = cfg["GRP"], cfg["NBANKS"], cfg["GCHUNK"]
    NPC, NG, BROWS = meta["NPC"], meta["NG"], meta["BROWS"]
    nidx_tot, nb_tot = meta["nidx_tot"], meta["nb_tot"]
    n_mm = meta["n_mm"]
    block_mms, mm_flags = meta["block_mms"], meta["mm_flags"]
    batches = meta["batches"]
    XK = _ceil_div(IN, 128)
    assert OUT == 128 and GRP == 128 and XK == 2
    last_w = NPC - (NG - 1) * GRP  # valid rows in the last group

    nc = bacc.Bacc(
        "TRN2", target_bir_lowering=False, debug=False, num_devices=C
    )

    # ---- external inputs ----
    xb = [
        nc.dram_tensor(f"xb{q}", [BROWS, IN], BF16, kind="ExternalInput")
        for q in range(NBANKS)
    ]
    wt = [
        nc.dram_tensor(f"wt{j}", [128, OUT], BF16, kind="ExternalInput")
        for j in range(XK)
    ]
    gidx_d = nc.dram_tensor("gidx", [128, nidx_tot // 16], I16, kind="ExternalInput")
    doff_d = nc.dram_tensor("doff", [128, n_mm], F32, kind="ExternalInput")
    wdeg_d = nc.dram_tensor("wdeg", [128, n_mm], F32, kind="ExternalInput")
    degi_d = nc.dram_tensor("degi", [128, NG], F32, kind="ExternalInput")
    bt_d = nc.dram_tensor("bt", [128, OUT], F32, kind="ExternalInput")
    iota_d = nc.dram_tensor("iota", [128, GRP], BF16, kind="ExternalInput")
    ident_d = nc.dram_tensor("ident", [128, 128], BF16, kind="ExternalInput")
    gm_d = nc.dram_tensor("gm", [1, OUT], F32, kind="ExternalInput")
    bb_d = nc.dram_tensor("bb", [1, OUT], F32, kind="ExternalInput")
    onesc_d = nc.dram_tensor("onesc", [128, 1], BF16, kind="ExternalInput")
    ones8_d = nc.dram_tensor("ones8", [8, 1], F32, kind="ExternalInput")
    onest_d = nc.dram_tensor("onest", [128, 1], BF16, kind="ExternalInput")
    onesr_d = nc.dram_tensor("onesr", [1, 128], F32, kind="ExternalInput")

    ypad_d = nc.dram_tensor("ypad", [NG * GRP, OUT], BF16, kind="ExternalOutput")

    with tile.TileContext(nc) as tc:
        with (
            tc.tile_pool(name="const", bufs=1) as cpool,
            tc.tile_pool(name="dram", bufs=1, space="DRAM") as dpool,
            tc.tile_pool(name="hrelu", bufs=1) as hpool,
            tc.tile_pool(name="gath", bufs=7) as gpool,
            tc.tile_pool(name="mpool", bufs=10) as mpool,
            tc.tile_pool(name="capool", bufs=6) as capool,
            tc.tile_pool(name="etmp", bufs=6) as epool,
            tc.tile_pool(name="gtmp", bufs=4) as gpool2,
            tc.tile_pool(name="psagg", bufs=1, space="PSUM") as pagg,
            tc.tile_pool(name="psaux", bufs=1, space="PSUM") as paux,
            tc.tile_pool(name="pstat", bufs=1, space="PSUM") as pspool,
        ):
            # ---- constants / small tiles ----
            iota_t = cpool.tile([128, GRP], BF16)
            ident_t = cpool.tile([128, 128], BF16)
            bt_t = cpool.tile([128, OUT], F32)
            degi_t = cpool.tile([128, NG], F32)
            ndst_t = cpool.tile([128, NG], F32)
            gm_t = cpool.tile([1, OUT], F32)
            bb_t = cpool.tile([1, OUT], F32)
            onesc_t = cpool.tile([128, 1], BF16)
            ones8_t = cpool.tile([8, 1], F32)
            onest_t = cpool.tile([128, 1], BF16)
            onesr_t = cpool.tile([1, 128], F32)
            gidx_t = cpool.tile([128, nidx_tot // 16], I16)
            doff_t = cpool.tile([128, n_mm], F32)
            wsrc_t = cpool.tile([128, n_mm], F32)
            wts = [cpool.tile([128, OUT], BF16, name=f"wt_s{j}") for j in range(XK)]

            # split the big index loads so the first gathers aren't gated on
            # the full-table DMA
            gsplit = min(nidx_tot // 16, 256)
            nc.sync.dma_start(gidx_t[:, :gsplit], gidx_d[:, :gsplit])
            if gsplit < nidx_tot // 16:
                nc.sync.dma_start(gidx_t[:, gsplit:], gidx_d[:, gsplit:])
            dsplit = min(n_mm, 128)
            nc.sync.dma_start(doff_t[:, :dsplit], doff_d[:, :dsplit])
            nc.sync.dma_start(wsrc_t[:, :dsplit], wdeg_d[:, :dsplit])
            if dsplit < n_mm:
                nc.sync.dma_start(doff_t[:, dsplit:], doff_d[:, dsplit:])
                nc.sync.dma_start(wsrc_t[:, dsplit:], wdeg_d[:, dsplit:])
            nc.sync.dma_start(iota_t[:], iota_d[:])
            nc.sync.dma_start(ident_t[:], ident_d[:])
            nc.sync.dma_start(bt_t[:], bt_d[:])
            nc.sync.dma_start(degi_t[:], degi_d[:])
            nc.sync.dma_start(gm_t[:], gm_d[:])
            nc.sync.dma_start(bb_t[:], bb_d[:])
            nc.sync.dma_start(onesc_t[:], onesc_d[:])
            nc.sync.dma_start(ones8_t[:], ones8_d[:])
            nc.sync.dma_start(onest_t[:], onest_d[:])
            nc.sync.dma_start(onesr_t[:], onesr_d[:])
            for j in range(XK):
                nc.sync.dma_start(wts[j][:], wt[j][:])

            # per-edge src norm: w = rsqrt(deg_out[src]) (pad slots carry 1.0);
            # two pieces so the first masks aren't gated on the full tile
            for c0, c1 in ((0, dsplit), (dsplit, n_mm)):
                if c0 >= c1:
                    continue
                nc.vector.reciprocal(wsrc_t[:, c0:c1], wsrc_t[:, c0:c1])
                nc.scalar.activation(
                    wsrc_t[:, c0:c1], wsrc_t[:, c0:c1],
                    mybir.ActivationFunctionType.Sqrt,
                )
            # dst norm: rsqrt(max(deg_in, 1)) tile-major [GRP, NG]
            nc.vector.tensor_scalar(
                ndst_t[:], degi_t[:], 1.0, None, op0=mybir.AluOpType.max
            )
            nc.vector.reciprocal(ndst_t[:], ndst_t[:])
            nc.scalar.activation(
                ndst_t[:], ndst_t[:], mybir.ActivationFunctionType.Sqrt
            )

            stats_in = dpool.tile([1, 2 * OUT], F32)
            _aspace = "Local" if cfg.get("NOCC") else "Shared"
            stats_out = dpool.tile([C, 2 * OUT], F32, addr_space=_aspace)

            hrelu_t = hpool.tile([128, NG, OUT], BF16)

            # ---- PSUM layout (8 banks x 2KB); accumulation-group zeroing is
            # bank-granular, so every concurrently-open chain gets its own
            # bank: 4x agg (GCHUNK groups in flight) + 1x transpose + 2x out
            # (alternating, WAR-tracked) + 1x BN stats (sum+sq as one chain).
            assert GC in (4, 5)
            # agg tiles carry a 512B tail used (bitcast bf16) as the per-group
            # transpose target, so each group's finish chain owns its own bank
            ps_agg = [
                pagg.tile([128, 2 * OUT + 128], F32, name=f"ps_agg{i}")
                for i in range(GC)
            ]
            n_out = 8 - GC - 1
            ps_out = [
                paux.tile([128, OUT], F32, name=f"ps_out{i}") for i in range(n_out)
            ]
            ps_stat = pspool.tile([1, 2, OUT], F32, name="ps_stat")

            # ---- stage D: gather x rows + one-hot matmul segmented sum ----
            def _finish_group(g):
                """Group g's PSUM agg is complete: apply W, relu, BN partials."""
                gi = g % GC
                cagg = capool.tile([128, 2 * OUT], BF16, tag="cagg")
                nc.scalar.activation(
                    cagg[:], ps_agg[gi][:, : 2 * OUT],
                    mybir.ActivationFunctionType.Copy,
                )
                # transpose into this group's own agg bank tail (bitcast bf16)
                # so concurrent groups' finish chains don't share a PSUM bank
                trv = ps_agg[gi][:, 2 * OUT :].bitcast(BF16)
                for h in range(2):
                    nc.tensor.matmul(
                        trv[:, h * OUT : (h + 1) * OUT],
                        cagg[:, h * OUT : (h + 1) * OUT],
                        ident_t[:],
                        is_transpose=True,
                        start=(h == 0),
                        stop=(h == 1),
                    )
                tagg = capool.tile([128, 2, OUT], BF16, tag="tagg")
                nc.scalar.activation(
                    tagg[:, :, :],
                    trv[:].rearrange("p (a f) -> p a f", a=2),
                    mybir.ActivationFunctionType.Copy,
                )
                po = ps_out[g % len(ps_out)]
                for jj in range(XK):
                    nc.tensor.matmul(
                        po[:],
                        tagg[:, jj, :],
                        wts[jj][:],
                        start=(jj == 0),
                        stop=(jj == XK - 1),
                    )
                # stage E: relu(out * ndst + b), BN partial sums
                tmp = epool.tile([128, OUT], F32, tag="etmp")
                nc.vector.scalar_tensor_tensor(
                    tmp[:],
                    po[:],
                    ndst_t[:, g : g + 1],
                    bt_t[:],
                    op0=mybir.AluOpType.mult,
                    op1=mybir.AluOpType.add,
                )
                nc.scalar.activation(
                    hrelu_t[:, g, :], tmp[:], mybir.ActivationFunctionType.Relu
                )
                ones = onesc_t if g < NG - 1 else onest_t
                nc.tensor.matmul(
                    ps_stat[:, 0, :],
                    ones[:],
                    hrelu_t[:, g, :],
                    start=(g == 0),
                    stop=False,
                )
                sq = epool.tile([128, OUT], BF16, tag="esq")
                nc.scalar.activation(
                    sq[:], hrelu_t[:, g, :], mybir.ActivationFunctionType.Square
                )
                nc.tensor.matmul(
                    ps_stat[:, 1, :],
                    ones[:],
                    sq[:],
                    start=False,
                    stop=(g == NG - 1),
                )

            bmax = max(nb for _, _, nb in batches)
            for bank, t0, nblk in batches:
                Gt = gpool.tile([128, bmax, IN], BF16, tag="G")
                nc.gpsimd.dma_gather(
                    Gt[:, :nblk, :],
                    xb[bank][:],
                    gidx_t[:, t0 * 8 : (t0 + nblk) * 8],
                    nblk * 128,
                    nblk * 128,
                    IN,
                    single_packet=False,
                )
                for j in range(nblk):
                    t = t0 + j
                    for m, g in block_mms[t]:
                      is_start, is_stop = mm_flags[m]
                      gi = g % GC
                      Mt = mpool.tile([128, GRP], BF16, tag="M")
                      nc.vector.tensor_scalar(
                          Mt[:],
                          iota_t[:],
                          doff_t[:, m : m + 1],
                          wsrc_t[:, m : m + 1],
                          op0=mybir.AluOpType.is_equal,
                          op1=mybir.AluOpType.mult,
                      )
                      nc.tensor.matmul(
                          ps_agg[gi][:, : 2 * OUT],
                          Mt[:],
                          Gt[:, j, :],
                          start=is_start,
                          stop=is_stop,
                      )
                      if is_stop:
                          _finish_group(g)

            # ---- stage F: AllReduce BN stats; build affine S/T tiles ----
            S_t = cpool.tile([128, OUT], BF16)
            T_t = cpool.tile([128, OUT], BF16)
            st_sb = cpool.tile([1, 2 * OUT], F32)
            nc.scalar.activation(
                st_sb[:], ps_stat[:].rearrange("p a f -> p (a f)"),
                mybir.ActivationFunctionType.Copy,
            )
            nc.sync.dma_start(stats_in[:], st_sb[:])
            if cfg.get("NOCC"):
                stats_out = stats_in  # single-core debug: sums are the totals
                st8 = st_sb
                ones8v = None
            else:
                # AllGather (no 1.875x reduce multiplier) + tiny local
                # ones-matmul reduction beats AllReduce on latency
                nc.gpsimd.collective_compute(
                    "AllGather",
                    mybir.AluOpType.bypass,
                    replica_groups=[list(range(C))],
                    ins=[stats_in[:]],
                    outs=[stats_out[:]],
                )
                st8 = cpool.tile([C, 2 * OUT], F32)
                nc.sync.dma_start(st8[:], stats_out[:])
                ones8v = ones8_t
            if ones8v is None:
                st_rb = cpool.tile([1, 2 * OUT], F32)
                nc.scalar.activation(
                    st_rb[:], st8[:], mybir.ActivationFunctionType.Copy
                )
                red_v = st_rb[:]
            else:
                ps_red = ps_stat[:].rearrange("p a f -> p (a f)")
                nc.tensor.matmul(ps_red, ones8v[:], st8[:], start=True, stop=True)
                red_v = ps_red

            mu = cpool.tile([1, OUT], F32)
            musq = cpool.tile([1, OUT], F32)
            ex2e = cpool.tile([1, OUT], F32)
            var = cpool.tile([1, OUT], F32)
            srow = cpool.tile([1, OUT], F32)
            trow = cpool.tile([1, OUT], F32)
            inv_n = 1.0 / float(N)
            # mu/ex2 straight off the reduction PSUM; fold 1/N and eps in
            nc.scalar.activation(
                mu[:], red_v[:, 0:OUT], mybir.ActivationFunctionType.Copy, scale=inv_n
            )
            nc.scalar.activation(
                ex2e[:],
                red_v[:, OUT : 2 * OUT],
                mybir.ActivationFunctionType.Copy,
                scale=inv_n,
                bias=float(cfg["EPS"]),
            )
            nc.scalar.activation(
                musq[:], mu[:], mybir.ActivationFunctionType.Square
            )
            # var + eps, then rsqrt via reciprocal+sqrt (ACT Rsqrt is banned
            # for accuracy)
            nc.vector.tensor_sub(var[:], ex2e[:], musq[:])
            nc.vector.reciprocal(var[:], var[:])
            nc.scalar.activation(
                var[:], var[:], mybir.ActivationFunctionType.Sqrt
            )
            nc.vector.tensor_mul(srow[:], gm_t[:], var[:])
            nc.vector.tensor_mul(trow[:], mu[:], srow[:])
            nc.vector.tensor_sub(trow[:], bb_t[:], trow[:])

            # reuse the (now idle) out banks for the S/T broadcast matmuls
            nc.tensor.matmul(ps_out[0][:], onesr_t[:], srow[:], start=True, stop=True)
            nc.scalar.activation(
                S_t[:], ps_out[0][:], mybir.ActivationFunctionType.Copy
            )
            nc.tensor.matmul(ps_out[1][:], onesr_t[:], trow[:], start=True, stop=True)
            nc.scalar.activation(
                T_t[:], ps_out[1][:], mybir.ActivationFunctionType.Copy
            )

            # ---- stage G: y = hrelu * S + T (in place, S/T broadcast along
            # the group axis), output DMA chunked to overlap with the DVE ----
            ypad_view = ypad_d[:].rearrange("(g p) f -> p g f", p=128)
            GOUT = 7
            for ci, c0 in enumerate(range(0, NG, GOUT)):
                c1 = min(c0 + GOUT, NG)
                S_b = S_t[:].rearrange("p (a f) -> p a f", a=1).to_broadcast(
                    (128, c1 - c0, OUT)
                )
                T_b = T_t[:].rearrange("p (a f) -> p a f", a=1).to_broadcast(
                    (128, c1 - c0, OUT)
                )
                eng = nc.gpsimd if ci >= 10 else nc.vector
                eng.tensor_mul(
                    hrelu_t[:, c0:c1, :], hrelu_t[:, c0:c1, :], S_b
                )
                eng.tensor_add(
                    hrelu_t[:, c0:c1, :], hrelu_t[:, c0:c1, :], T_b
                )
                nc.sync.dma_start(
                    ypad_view[:, c0:c1, :], hrelu_t[:, c0:c1, :]
                )

    nc.compile()
    return nc


def kernel(x, src, dst, W, b, gamma, beta):
    global LAST_RESULTS
    cfg = CFG
    N, E, IN, OUT, C = cfg["N"], cfg["E"], cfg["IN"], cfg["OUT"], cfg["NCORES"]
    GRP = cfg["GRP"]
    assert x.shape == (N, IN) and W.shape == (IN, OUT)
    assert src.shape == (E,) and dst.shape == (E,)

    meta, gidx_cores, dstoff_cores, wdeg_cores = _preprocess(cfg, src, dst)
    NPC, NG, BROWS = meta["NPC"], meta["NG"], meta["BROWS"]
    XK = _ceil_div(IN, 128)
    last_w = NPC - (NG - 1) * GRP

    nc = _build_nc(cfg, meta)

    import ml_dtypes

    x_bf = np.asarray(x, np.float32).astype(ml_dtypes.bfloat16)
    Wn = np.asarray(W, np.float32)

    iota = np.tile(
        np.arange(GRP, dtype=np.float32)[None, :], (128, 1)
    ).astype(ml_dtypes.bfloat16)
    ident = np.eye(128, dtype=np.float32).astype(ml_dtypes.bfloat16)
    bt = np.tile(np.asarray(b, np.float32)[None, :], (128, 1))
    onesc = np.ones((128, 1), np.float32).astype(ml_dtypes.bfloat16)
    ones8 = np.ones((8, 1), np.float32)
    onest = np.zeros((128, 1), np.float32)
    onest[:last_w] = 1.0
    onest = onest.astype(ml_dtypes.bfloat16)
    onesr = np.ones((1, 128), np.float32)
    gm = np.asarray(gamma, np.float32)[None, :]
    bb = np.asarray(beta, np.float32)[None, :]
    xbanks = [
        np.ascontiguousarray(x_bf[q * BROWS : (q + 1) * BROWS])
        for q in range(cfg["NBANKS"])
    ]
    wtiles = [
        np.ascontiguousarray(Wn[j * 128 : (j + 1) * 128, :]).astype(
            ml_dtypes.bfloat16
        )
        for j in range(XK)
    ]

    in_maps = []
    for k in range(C):
        im = {
            "gidx": gidx_cores[k],
            "doff": dstoff_cores[k],
            "wdeg": wdeg_cores[k],
            "degi": _tile_major(
                meta["deg_in"][k * NPC : (k + 1) * NPC], NG, GRP, np.float32(1.0)
            ),
            "bt": bt,
            "iota": iota,
            "ident": ident,
            "gm": gm,
            "bb": bb,
            "onesc": onesc,
            "ones8": ones8,
            "onest": onest,
            "onesr": onesr,
        }
        for q in range(cfg["NBANKS"]):
            im[f"xb{q}"] = xbanks[q]
        for j in range(XK):
            im[f"wt{j}"] = wtiles[j]
        in_maps.append(im)

    if cfg.get("SIM"):
        from concourse.bass_interp import MultiCoreSim

        sim = MultiCoreSim(nc, num_cores=C)
        for k, core_sim in sim.cores.items():
            for name, val in in_maps[k].items():
                core_sim.tensor(name)[:] = val
        sim.simulate()
        y = np.empty((N, OUT), np.float32)
        perm = meta["perm"]
        for k in range(C):
            y[perm[k * NPC : (k + 1) * NPC]] = np.asarray(
                sim.cores[k].tensor("ypad")[:NPC], dtype=np.float32
            )
        return y

    global LAST_NC, LAST_RUN_S
    LAST_NC = nc
    import time as _time

    _t0 = _time.time()
    res = bass_utils.run_bass_kernel_spmd(
        nc,
        in_maps,
        core_ids=list(range(C)),
        trace=cfg.get("TRACE", False),
    )
    LAST_RUN_S = _time.time() - _t0
    LAST_RESULTS = res

    y = np.empty((N, OUT), np.float32)
    perm = meta["perm"]
    for k in range(C):
        y[perm[k * NPC : (k + 1) * NPC]] = np.asarray(
            res.results[k]["ypad"][:NPC], dtype=np.float32
        )
    return y


# revision 13
# speedup vs baseline: 1.3900x; 1.0173x over previous
"""GCN block (GraphConv + BatchNorm1d + ReLU) on 8 Trainium2 NeuronCores.

Strategy — "gather x, apply W after aggregation":

By linearity, agg[dst] = sum_e norm_src[src_e] * (x[src_e] @ W)
                       = (sum_e norm_src[src_e] * x[src_e]) @ W.
So instead of computing h = x@W per shard and AllGather-ing the h table
across cores (collectives dominated the v1 timeline), every core receives
the FULL x (bf16, row-major) in its own HBM and directly dma_gathers raw x
rows for its edges. No AllGather of features at all, and gathers start at
t=0. x rows are 256 bf16 = 512 B, which also clears the <512 B
small-descriptor DMA penalty that h rows (128 bf16 = 256 B) would pay.

Per core k (owns a permuted 1/8 of the dst nodes):
  1. For each 128-edge block (edges bucketed by (src-bank, dst-group)),
     gather x[src] rows (batched dma_gather, bf16) and segment-sum them with
     one-hot matmuls Mw^T @ G accumulated in PSUM (4 groups in flight, one
     PSUM bank per open accumulation chain). The mask is built on DVE as
     (iota == dstoff) * w_e with w_e = rsqrt(deg_out[src_e]) — norm_src is
     folded into the aggregation.
  2. Per finished dst group: agg [128,256] PSUM -> bf16 SBUF -> PE transpose
     (into a bf16-bitcast tail of the group's own agg bank) -> aggT_j^T @ W_j
     -> out [128, OUT] PSUM; relu(out * rsqrt(clip(deg_in,1)) + b) -> bf16;
     BN partial sums via ones-matmuls into a single PSUM stats chain.
  3. AllGather the [1, 2*OUT] per-core BN sums (cheaper than AllReduce),
     reduce with a ones8 matmul, build the affine S/T tiles, then
     y = hrelu * S + T in-place (S/T broadcast along the group axis, split
     across DVE and GpSimd) with the bf16 output DMA chunk-interleaved.
     The host upcasts y to fp32 (quantization ~0.4% rms << 2e-2 gate).

Host-side work is limited to integer index bookkeeping (edge bucketing,
degree counting, node permutation) and layout transforms (bf16 casts, int16
gather indices). All floating-point math runs on device.

Edges are bucketed by src bank (4 banks of N/4 rows) because dma_gather
indices are int16 (< 32768). Bucket capacities are the exact max over the 8
cores (shared SPMD structure; one NEFF serves every core); dst nodes are
permuted so cores\' same-position buckets have near-identical per-bank edge
counts (lexsort by per-bank degree vector, dealt round-robin), which cuts
the max-over-cores padding to ~3.6%. 128-edge blocks may straddle bucket
boundaries -> one matmul per (block, overlapped group). Pad slots gather
row 0 and carry dst offset 255 -> their one-hot column is all zeros, so
they contribute exactly 0. Banks 0 and 3 get >= 1 slot per group so every
group\'s accumulation chain starts in bank 0 and stops in bank 3, keeping
the BN-stat chain\'s start first / stop last in program order.
"""
import math
import os
import sys

sys.path.insert(0, "/opt/trn_rl_repo")

import numpy as np

import concourse.bacc as bacc
import concourse.bass as bass
import concourse.mybir as mybir
import concourse.tile as tile
from concourse import bass_utils

F32 = mybir.dt.float32
BF16 = mybir.dt.bfloat16
I16 = mybir.dt.int16

CFG = dict(
    N=100000,
    E=1600000,
    IN=256,
    OUT=128,
    NCORES=8,
    GRP=128,          # dst nodes per segment group (= psum partition dim)
    NBANKS=4,         # src banks (bank rows must stay < 32768 for int16 idx)
    GCHUNK=4,         # dst groups in flight (1 PSUM bank per open accum chain)
    BATCH_BLOCKS=17,  # gather batch size in 128-edge blocks
    EPS=1e-5,
    TRACE=False,
)

LAST_RESULTS = None  # set by kernel() for test harness introspection
LAST_NC = None
LAST_RUN_S = None


def _ceil_div(a, b):
    return (a + b - 1) // b


def _wrap16(idx, ncols):
    """int16 idx list -> [128, ncols] tile: idx i at [i%16, i//16], replicated
    8x across the 16-partition groups (one copy per GpSimd Q7 core)."""
    n = idx.shape[0]
    assert n == ncols * 16
    w = np.ascontiguousarray(idx.reshape(ncols, 16).T)
    return np.tile(w, (8, 1))


def _preprocess(cfg, src, dst):
    """Bucket edges by (owner core, src bank, dst group); build per-core
    gather-index / dst-offset / src-degree arrays and the shared block
    structure."""
    N, E = cfg["N"], cfg["E"]
    C, NBANKS, GRP = cfg["NCORES"], cfg["NBANKS"], cfg["GRP"]
    NPC = N // C
    NG = _ceil_div(NPC, GRP)
    assert N % NBANKS == 0
    BROWS = N // NBANKS            # rows per x bank (gather source table)
    assert BROWS < 32768

    src = src.astype(np.int64)
    dst = dst.astype(np.int64)
    deg_out = np.bincount(src, minlength=N).astype(np.float32)
    deg_in = np.bincount(dst, minlength=N).astype(np.float32)
    bank = src // BROWS

    # permute dst nodes into (core, group, slot) positions so that the 8
    # cores' group-i buckets have near-identical per-bank in-edge counts --
    # shrinks the SPMD max-over-cores padding of the shared block structure.
    # Nodes sorted by per-bank count vector are dealt round-robin across
    # cores at each (group, slot).
    nbc = np.zeros((N, NBANKS), np.int64)
    for b in range(NBANKS):
        nbc[:, b] = np.bincount(dst[bank == b], minlength=N)
    node_order = np.lexsort(tuple(-nbc[:, b] for b in range(NBANKS - 1, -1, -1)))
    allpos = np.arange(N)
    deal_order = np.lexsort((allpos // NPC, allpos % GRP, (allpos % NPC) // GRP))
    perm = np.empty(N, np.int64)      # new position -> original node
    perm[deal_order] = node_order
    pos = np.empty(N, np.int64)       # original node -> new position
    pos[perm] = allpos

    pdst = pos[dst]
    owner = pdst // NPC
    grp = (pdst % NPC) // GRP
    key = (owner * NBANKS + bank) * NG + grp
    order = np.argsort(key, kind="stable")
    s_src = src[order]
    s_pdst = pos[dst[order]]
    s_key = key[order]

    counts = np.bincount(key, minlength=C * NBANKS * NG).reshape(C, NBANKS, NG)
    # bucket capacity: exact max over cores (shared SPMD structure); >= 1 in
    # banks 0/3 so every group has a first (bank-0) and last (bank-3) matmul
    P = counts.max(axis=0)  # [NBANKS, NG]
    P[0] = np.maximum(P[0], 1)
    P[NBANKS - 1] = np.maximum(P[NBANKS - 1], 1)

    # stream order: group-chunks outer, banks inner; a group's PSUM slot is
    # live across all banks of its chunk (accumulated with start/stop).
    # Segment (chunk, bank) = that chunk's buckets concatenated, padded to a
    # multiple of 128; 128-edge blocks may straddle bucket (group) boundaries
    # -> one matmul per (block, overlapped group).
    GC = cfg["GCHUNK"]
    chunks = [list(range(c, min(c + GC, NG))) for c in range(0, NG, GC)]
    run_off = np.zeros((NBANKS, NG), np.int64)
    segments = []  # (bank, seg_start_slot, seg_nblocks)
    pos = 0
    for ch in chunks:
        for b in range(NBANKS):
            seg0 = pos
            for g in ch:
                run_off[b, g] = pos
                pos += P[b, g]
            pos = ((pos + 127) // 128) * 128  # segment tail pad
            segments.append((b, seg0, (pos - seg0) // 128))
    nidx_tot = pos
    nb_tot = nidx_tot // 128

    # per block: list of matmuls (mm_col, group); per group: first/last mm id
    block_mms = [[] for _ in range(nb_tot)]
    n_mm = 0
    mm_of_group = {}
    for ch in chunks:
        for b in range(NBANKS):
            for g in ch:
                o0, o1 = run_off[b, g], run_off[b, g] + P[b, g]
                for t in range(o0 // 128, (o1 - 1) // 128 + 1):
                    block_mms[t].append((n_mm, g))
                    mm_of_group.setdefault(g, []).append(n_mm)
                    n_mm += 1
    mm_flags = {}
    for g, mms in mm_of_group.items():
        for m in mms:
            mm_flags[m] = (m == mms[0], m == mms[-1])

    # boundaries of each (k, b, g) bucket in the sorted edge stream
    bkeys = (np.arange(C)[:, None, None] * NBANKS + np.arange(NBANKS)[None, :, None]) * NG + np.arange(NG)[None, None, :]
    starts = np.searchsorted(s_key, bkeys.ravel()).reshape(C, NBANKS, NG)
    ends = np.searchsorted(s_key, bkeys.ravel(), side="right").reshape(C, NBANKS, NG)

    gidx_cores = []
    dstoff_cores = []
    wdeg_cores = []
    for k in range(C):
        gidx = np.zeros(nidx_tot, np.int16)
        doff = np.full((n_mm, 128), 255.0, np.float32)
        wdeg = np.ones((n_mm, 128), np.float32)
        # fill gather indices per bucket (pad slots keep row 0)
        for b in range(NBANKS):
            for g in range(NG):
                s, e = starts[k, b, g], ends[k, b, g]
                cnt = e - s
                if cnt == 0:
                    continue
                p0 = run_off[b, g]
                gidx[p0 : p0 + cnt] = (s_src[s:e] % BROWS).astype(np.int16)
        # fill per-matmul mask columns: rows = this core's real edges of the
        # matmul's group that fall inside the block's 128-slot window
        for ch in chunks:
            for b in range(NBANKS):
                for g in ch:
                    s, e = starts[k, b, g], ends[k, b, g]
                    cnt = e - s
                    o0 = run_off[b, g]
                    bucket_mms = [
                        (m, t)
                        for t in range(o0 // 128, (o0 + P[b, g] - 1) // 128 + 1)
                        for (m, gg) in block_mms[t]
                        if gg == g
                    ]
                    if cnt == 0:
                        continue
                    dvals = ((s_pdst[s:e] % NPC) - g * GRP).astype(np.float32)
                    wvals = deg_out[s_src[s:e]]
                    for m, t in bucket_mms:
                        w0 = t * 128
                        lo = max(o0, w0)
                        hi = min(o0 + cnt, w0 + 128)
                        if lo >= hi:
                            continue
                        rows = np.arange(lo - w0, hi - w0)
                        doff[m, rows] = dvals[lo - o0 : hi - o0]
                        wdeg[m, rows] = wvals[lo - o0 : hi - o0]
        gidx_cores.append(_wrap16(gidx, nidx_tot // 16))
        dstoff_cores.append(np.ascontiguousarray(doff.T))
        wdeg_cores.append(np.ascontiguousarray(wdeg.T))

    # gather batches: split segments longer than BATCH_BLOCKS
    batches = []  # (bank, first_block, n_blocks)
    for b, seg0, nblk in segments:
        t = seg0 // 128
        left = nblk
        while left > 0:
            n = min(left, cfg["BATCH_BLOCKS"])
            batches.append((b, t, n))
            t += n
            left -= n

    meta = dict(
        NPC=NPC,
        NG=NG,
        BROWS=BROWS,
        nidx_tot=nidx_tot,
        nb_tot=nb_tot,
        n_mm=n_mm,
        block_mms=block_mms,
        mm_flags=mm_flags,
        batches=batches,
        deg_in=deg_in[perm],
        perm=perm,
    )
    return meta, gidx_cores, dstoff_cores, wdeg_cores


def _tile_major(vec, NG, GRP, pad_val):
    """[NPC] -> [GRP, NG]: entry (p, m) = vec[m*GRP + p], padded."""
    out = np.full((NG * GRP,), pad_val, vec.dtype)
    out[: vec.shape[0]] = vec
    return np.ascontiguousarray(out.reshape(NG, GRP).T)


def _build_nc(cfg, meta):
    N, IN, OUT, C = cfg["N"], cfg["IN"], cfg["OUT"], cfg["NCORES"]
    GRP, NBANKS, GC = cfg["GRP"], cfg["NBANKS"], cfg["GCHUNK"]
    NPC, NG, BROWS = meta["NPC"], meta["NG"], meta["BROWS"]
    nidx_tot, nb_tot = meta["nidx_tot"], meta["nb_tot"]
    n_mm = meta["n_mm"]
    block_mms, mm_flags = meta["block_mms"], meta["mm_flags"]
    batches = meta["batches"]
    XK = _ceil_div(IN, 128)
    assert OUT == 128 and GRP == 128 and XK == 2
    last_w = NPC - (NG - 1) * GRP  # valid rows in the last group

    nc = bacc.Bacc(
        "TRN2", target_bir_lowering=False, debug=False, num_devices=C
    )

    # ---- external inputs ----
    xb = [
        nc.dram_tensor(f"xb{q}", [BROWS, IN], BF16, kind="ExternalInput")
        for q in range(NBANKS)
    ]
    wt = [
        nc.dram_tensor(f"wt{j}", [128, OUT], BF16, kind="ExternalInput")
        for j in range(XK)
    ]
    gidx_d = nc.dram_tensor("gidx", [128, nidx_tot // 16], I16, kind="ExternalInput")
    doff_d = nc.dram_tensor("doff", [128, n_mm], F32, kind="ExternalInput")
    wdeg_d = nc.dram_tensor("wdeg", [128, n_mm], F32, kind="ExternalInput")
    degi_d = nc.dram_tensor("degi", [128, NG], F32, kind="ExternalInput")
    bt_d = nc.dram_tensor("bt", [128, OUT], F32, kind="ExternalInput")
    iota_d = nc.dram_tensor("iota", [128, GRP], BF16, kind="ExternalInput")
    ident_d = nc.dram_tensor("ident", [128, 128], BF16, kind="ExternalInput")
    gm_d = nc.dram_tensor("gm", [1, OUT], F32, kind="ExternalInput")
    bb_d = nc.dram_tensor("bb", [1, OUT], F32, kind="ExternalInput")
    onesc_d = nc.dram_tensor("onesc", [128, 1], BF16, kind="ExternalInput")
    ones8_d = nc.dram_tensor("ones8", [8, 1], F32, kind="ExternalInput")
    onest_d = nc.dram_tensor("onest", [128, 1], BF16, kind="ExternalInput")
    onesr_d = nc.dram_tensor("onesr", [1, 128], F32, kind="ExternalInput")

    ypad_d = nc.dram_tensor("ypad", [NG * GRP, OUT], BF16, kind="ExternalOutput")

    with tile.TileContext(nc) as tc:
        with (
            tc.tile_pool(name="const", bufs=1) as cpool,
            tc.tile_pool(name="dram", bufs=1, space="DRAM") as dpool,
            tc.tile_pool(name="hrelu", bufs=1) as hpool,
            tc.tile_pool(name="gath", bufs=7) as gpool,
            tc.tile_pool(name="mpool", bufs=10) as mpool,
            tc.tile_pool(name="capool", bufs=6) as capool,
            tc.tile_pool(name="etmp", bufs=6) as epool,
            tc.tile_pool(name="gtmp", bufs=4) as gpool2,
            tc.tile_pool(name="psagg", bufs=1, space="PSUM") as pagg,
            tc.tile_pool(name="psaux", bufs=1, space="PSUM") as paux,
            tc.tile_pool(name="pstat", bufs=1, space="PSUM") as pspool,
        ):
            # ---- constants / small tiles ----
            iota_t = cpool.tile([128, GRP], BF16)
            ident_t = cpool.tile([128, 128], BF16)
            bt_t = cpool.tile([128, OUT], F32)
            degi_t = cpool.tile([128, NG], F32)
            ndst_t = cpool.tile([128, NG], F32)
            gm_t = cpool.tile([1, OUT], F32)
            bb_t = cpool.tile([1, OUT], F32)
            onesc_t = cpool.tile([128, 1], BF16)
            ones8_t = cpool.tile([8, 1], F32)
            onest_t = cpool.tile([128, 1], BF16)
            onesr_t = cpool.tile([1, 128], F32)
            gidx_t = cpool.tile([128, nidx_tot // 16], I16)
            doff_t = cpool.tile([128, n_mm], F32)
            wsrc_t = cpool.tile([128, n_mm], F32)
            wts = [cpool.tile([128, OUT], BF16, name=f"wt_s{j}") for j in range(XK)]

            # split the big index loads so the first gathers aren't gated on
            # the full-table DMA
            gsplit = min(nidx_tot // 16, 256)
            nc.sync.dma_start(gidx_t[:, :gsplit], gidx_d[:, :gsplit])
            if gsplit < nidx_tot // 16:
                nc.sync.dma_start(gidx_t[:, gsplit:], gidx_d[:, gsplit:])
            dsplit = min(n_mm, 128)
            nc.sync.dma_start(doff_t[:, :dsplit], doff_d[:, :dsplit])
            nc.sync.dma_start(wsrc_t[:, :dsplit], wdeg_d[:, :dsplit])
            if dsplit < n_mm:
                nc.sync.dma_start(doff_t[:, dsplit:], doff_d[:, dsplit:])
                nc.sync.dma_start(wsrc_t[:, dsplit:], wdeg_d[:, dsplit:])
            nc.sync.dma_start(iota_t[:], iota_d[:])
            nc.sync.dma_start(ident_t[:], ident_d[:])
            nc.sync.dma_start(bt_t[:], bt_d[:])
            nc.sync.dma_start(degi_t[:], degi_d[:])
            nc.sync.dma_start(gm_t[:], gm_d[:])
            nc.sync.dma_start(bb_t[:], bb_d[:])
            nc.sync.dma_start(onesc_t[:], onesc_d[:])
            nc.sync.dma_start(ones8_t[:], ones8_d[:])
            nc.sync.dma_start(onest_t[:], onest_d[:])
            nc.sync.dma_start(onesr_t[:], onesr_d[:])
            for j in range(XK):
                nc.sync.dma_start(wts[j][:], wt[j][:])

            # per-edge src norm: w = rsqrt(deg_out[src]) (pad slots carry 1.0);
            # two pieces so the first masks aren't gated on the full tile
            for c0, c1 in ((0, dsplit), (dsplit, n_mm)):
                if c0 >= c1:
                    continue
                nc.vector.reciprocal(wsrc_t[:, c0:c1], wsrc_t[:, c0:c1])
                nc.scalar.activation(
                    wsrc_t[:, c0:c1], wsrc_t[:, c0:c1],
                    mybir.ActivationFunctionType.Sqrt,
                )
            # dst norm: rsqrt(max(deg_in, 1)) tile-major [GRP, NG]
            nc.vector.tensor_scalar(
                ndst_t[:], degi_t[:], 1.0, None, op0=mybir.AluOpType.max
            )
            nc.vector.reciprocal(ndst_t[:], ndst_t[:])
            nc.scalar.activation(
                ndst_t[:], ndst_t[:], mybir.ActivationFunctionType.Sqrt
            )

            stats_in = dpool.tile([1, 2 * OUT], F32)
            _aspace = "Local" if cfg.get("NOCC") else "Shared"
            stats_out = dpool.tile([C, 2 * OUT], F32, addr_space=_aspace)

            hrelu_t = hpool.tile([128, NG, OUT], BF16)

            # ---- PSUM layout (8 banks x 2KB); accumulation-group zeroing is
            # bank-granular, so every concurrently-open chain gets its own
            # bank: 4x agg (GCHUNK groups in flight) + 1x transpose + 2x out
            # (alternating, WAR-tracked) + 1x BN stats (sum+sq as one chain).
            assert GC in (4, 5)
            # agg tiles carry a 512B tail used (bitcast bf16) as the per-group
            # transpose target, so each group's finish chain owns its own bank
            ps_agg = [
                pagg.tile([128, 2 * OUT + 128], F32, name=f"ps_agg{i}")
                for i in range(GC)
            ]
            n_out = 8 - GC - 1
            ps_out = [
                paux.tile([128, OUT], F32, name=f"ps_out{i}") for i in range(n_out)
            ]
            ps_stat = pspool.tile([1, 2, OUT], F32, name="ps_stat")

            # ---- stage D: gather x rows + one-hot matmul segmented sum ----
            def _finish_group(g):
                """Group g's PSUM agg is complete: apply W, relu, BN partials."""
                gi = g % GC
                cagg = capool.tile([128, 2 * OUT], BF16, tag="cagg")
                nc.scalar.activation(
                    cagg[:], ps_agg[gi][:, : 2 * OUT],
                    mybir.ActivationFunctionType.Copy,
                )
                # transpose into this group's own agg bank tail (bitcast bf16)
                # so concurrent groups' finish chains don't share a PSUM bank
                trv = ps_agg[gi][:, 2 * OUT :].bitcast(BF16)
                for h in range(2):
                    nc.tensor.matmul(
                        trv[:, h * OUT : (h + 1) * OUT],
                        cagg[:, h * OUT : (h + 1) * OUT],
                        ident_t[:],
                        is_transpose=True,
                        start=(h == 0),
                        stop=(h == 1),
                    )
                tagg = capool.tile([128, 2, OUT], BF16, tag="tagg")
                nc.scalar.activation(
                    tagg[:, :, :],
                    trv[:].rearrange("p (a f) -> p a f", a=2),
                    mybir.ActivationFunctionType.Copy,
                )
                po = ps_out[g % len(ps_out)]
                for jj in range(XK):
                    nc.tensor.matmul(
                        po[:],
                        tagg[:, jj, :],
                        wts[jj][:],
                        start=(jj == 0),
                        stop=(jj == XK - 1),
                    )
                # stage E: relu(out * ndst + b), BN partial sums
                tmp = epool.tile([128, OUT], F32, tag="etmp")
                nc.vector.scalar_tensor_tensor(
                    tmp[:],
                    po[:],
                    ndst_t[:, g : g + 1],
                    bt_t[:],
                    op0=mybir.AluOpType.mult,
                    op1=mybir.AluOpType.add,
                )
                nc.scalar.activation(
                    hrelu_t[:, g, :], tmp[:], mybir.ActivationFunctionType.Relu
                )
                ones = onesc_t if g < NG - 1 else onest_t
                nc.tensor.matmul(
                    ps_stat[:, 0, :],
                    ones[:],
                    hrelu_t[:, g, :],
                    start=(g == 0),
                    stop=False,
                )
                sq = epool.tile([128, OUT], BF16, tag="esq")
                nc.scalar.activation(
                    sq[:], hrelu_t[:, g, :], mybir.ActivationFunctionType.Square
                )
                nc.tensor.matmul(
                    ps_stat[:, 1, :],
                    ones[:],
                    sq[:],
                    start=False,
                    stop=(g == NG - 1),
                )

            bmax = max(nb for _, _, nb in batches)
            for bank, t0, nblk in batches:
                Gt = gpool.tile([128, bmax, IN], BF16, tag="G")
                nc.gpsimd.dma_gather(
                    Gt[:, :nblk, :],
                    xb[bank][:],
                    gidx_t[:, t0 * 8 : (t0 + nblk) * 8],
                    nblk * 128,
                    nblk * 128,
                    IN,
                    single_packet=False,
                )
                for j in range(nblk):
                    t = t0 + j
                    for m, g in block_mms[t]:
                      is_start, is_stop = mm_flags[m]
                      gi = g % GC
                      Mt = mpool.tile([128, GRP], BF16, tag="M")
                      nc.vector.tensor_scalar(
                          Mt[:],
                          iota_t[:],
                          doff_t[:, m : m + 1],
                          wsrc_t[:, m : m + 1],
                          op0=mybir.AluOpType.is_equal,
                          op1=mybir.AluOpType.mult,
                      )
                      nc.tensor.matmul(
                          ps_agg[gi][:, : 2 * OUT],
                          Mt[:],
                          Gt[:, j, :],
                          start=is_start,
                          stop=is_stop,
                      )
                      if is_stop:
                          _finish_group(g)

            # ---- stage F: AllReduce BN stats; build affine S/T tiles ----
            S_t = cpool.tile([128, OUT], BF16)
            T_t = cpool.tile([128, OUT], BF16)
            st_sb = cpool.tile([1, 2 * OUT], F32)
            nc.scalar.activation(
                st_sb[:], ps_stat[:].rearrange("p a f -> p (a f)"),
                mybir.ActivationFunctionType.Copy,
            )
            nc.sync.dma_start(stats_in[:], st_sb[:])
            if cfg.get("NOCC"):
                stats_out = stats_in  # single-core debug: sums are the totals
                st8 = st_sb
                ones8v = None
            else:
                # AllGather (no 1.875x reduce multiplier) + tiny local
                # ones-matmul reduction beats AllReduce on latency
                nc.gpsimd.collective_compute(
                    "AllGather",
                    mybir.AluOpType.bypass,
                    replica_groups=[list(range(C))],
                    ins=[stats_in[:]],
                    outs=[stats_out[:]],
                )
                st8 = cpool.tile([C, 2 * OUT], F32)
                nc.sync.dma_start(st8[:], stats_out[:])
                ones8v = ones8_t
            if ones8v is None:
                st_rb = cpool.tile([1, 2 * OUT], F32)
                nc.scalar.activation(
                    st_rb[:], st8[:], mybir.ActivationFunctionType.Copy
                )
                red_v = st_rb[:]
            else:
                ps_red = ps_stat[:].rearrange("p a f -> p (a f)")
                nc.tensor.matmul(ps_red, ones8v[:], st8[:], start=True, stop=True)
                red_v = ps_red

            mu = cpool.tile([1, OUT], F32)
            musq = cpool.tile([1, OUT], F32)
            ex2e = cpool.tile([1, OUT], F32)
            var = cpool.tile([1, OUT], F32)
            srow = cpool.tile([1, OUT], F32)
            trow = cpool.tile([1, OUT], F32)
            inv_n = 1.0 / float(N)
            # mu/ex2 straight off the reduction PSUM; fold 1/N and eps in
            nc.scalar.activation(
                mu[:], red_v[:, 0:OUT], mybir.ActivationFunctionType.Copy, scale=inv_n
            )
            nc.scalar.activation(
                ex2e[:],
                red_v[:, OUT : 2 * OUT],
                mybir.ActivationFunctionType.Copy,
                scale=inv_n,
                bias=float(cfg["EPS"]),
            )
            nc.scalar.activation(
                musq[:], mu[:], mybir.ActivationFunctionType.Square
            )
            # var + eps, then rsqrt via reciprocal+sqrt (ACT Rsqrt is banned
            # for accuracy)
            nc.vector.tensor_sub(var[:], ex2e[:], musq[:])
            nc.vector.reciprocal(var[:], var[:])
            nc.scalar.activation(
                var[:], var[:], mybir.ActivationFunctionType.Sqrt
            )
            nc.vector.tensor_mul(srow[:], gm_t[:], var[:])
            nc.vector.tensor_mul(trow[:], mu[:], srow[:])
            nc.vector.tensor_sub(trow[:], bb_t[:], trow[:])

            # reuse the (now idle) out banks for the S/T broadcast matmuls
            nc.tensor.matmul(ps_out[0][:], onesr_t[:], srow[:], start=True, stop=True)
            nc.scalar.activation(
                S_t[:], ps_out[0][:], mybir.ActivationFunctionType.Copy
            )
            nc.tensor.matmul(ps_out[1][:], onesr_t[:], trow[:], start=True, stop=True)
            nc.scalar.activation(
                T_t[:], ps_out[1][:], mybir.ActivationFunctionType.Copy
            )

            # ---- stage G: y = hrelu * S + T (in place, S/T broadcast along
            # the group axis), output DMA chunked to overlap with the DVE ----
            ypad_view = ypad_d[:].rearrange("(g p) f -> p g f", p=128)
            GOUT = 7
            for ci, c0 in enumerate(range(0, NG, GOUT)):
                c1 = min(c0 + GOUT, NG)
                S_b = S_t[:].rearrange("p (a f) -> p a f", a=1).to_broadcast(
                    (128, c1 - c0, OUT)
                )
                T_b = T_t[:].rearrange("p (a f) -> p a f", a=1).to_broadcast(
                    (128, c1 - c0, OUT)
                )
                eng = nc.gpsimd if ci >= 10 else nc.vector
                eng.tensor_mul(
                    hrelu_t[:, c0:c1, :], hrelu_t[:, c0:c1, :], S_b
                )
                eng.tensor_add(
                    hrelu_t[:, c0:c1, :], hrelu_t[:, c0:c1, :], T_b
                )
                nc.sync.dma_start(
                    ypad_view[:, c0:c1, :], hrelu_t[:, c0:c1, :]
                )

    nc.compile()
    return nc


def kernel(x, src, dst, W, b, gamma, beta):
    global LAST_RESULTS
    cfg = CFG
    N, E, IN, OUT, C = cfg["N"], cfg["E"], cfg["IN"], cfg["OUT"], cfg["NCORES"]
    GRP = cfg["GRP"]
    assert x.shape == (N, IN) and W.shape == (IN, OUT)
    assert src.shape == (E,) and dst.shape == (E,)

    meta, gidx_cores, dstoff_cores, wdeg_cores = _preprocess(cfg, src, dst)
    NPC, NG, BROWS = meta["NPC"], meta["NG"], meta["BROWS"]
    XK = _ceil_div(IN, 128)
    last_w = NPC - (NG - 1) * GRP

    nc = _build_nc(cfg, meta)

    import ml_dtypes

    x_bf = np.asarray(x, np.float32).astype(ml_dtypes.bfloat16)
    Wn = np.asarray(W, np.float32)

    iota = np.tile(
        np.arange(GRP, dtype=np.float32)[None, :], (128, 1)
    ).astype(ml_dtypes.bfloat16)
    ident = np.eye(128, dtype=np.float32).astype(ml_dtypes.bfloat16)
    bt = np.tile(np.asarray(b, np.float32)[None, :], (128, 1))
    onesc = np.ones((128, 1), np.float32).astype(ml_dtypes.bfloat16)
    ones8 = np.ones((8, 1), np.float32)
    onest = np.zeros((128, 1), np.float32)
    onest[:last_w] = 1.0
    onest = onest.astype(ml_dtypes.bfloat16)
    onesr = np.ones((1, 128), np.float32)
    gm = np.asarray(gamma, np.float32)[None, :]
    bb = np.asarray(beta, np.float32)[None, :]
    xbanks = [
        np.ascontiguousarray(x_bf[q * BROWS : (q + 1) * BROWS])
        for q in range(cfg["NBANKS"])
    ]
    wtiles = [
        np.ascontiguousarray(Wn[j * 128 : (j + 1) * 128, :]).astype(
            ml_dtypes.bfloat16
        )
        for j in range(XK)
    ]

    in_maps = []
    for k in range(C):
        im = {
            "gidx": gidx_cores[k],
            "doff": dstoff_cores[k],
            "wdeg": wdeg_cores[k],
            "degi": _tile_major(
                meta["deg_in"][k * NPC : (k + 1) * NPC], NG, GRP, np.float32(1.0)
            ),
            "bt": bt,
            "iota": iota,
            "ident": ident,
            "gm": gm,
            "bb": bb,
            "onesc": onesc,
            "ones8": ones8,
            "onest": onest,
            "onesr": onesr,
        }
        for q in range(cfg["NBANKS"]):
            im[f"xb{q}"] = xbanks[q]
        for j in range(XK):
            im[f"wt{j}"] = wtiles[j]
        in_maps.append(im)

    if cfg.get("SIM"):
        from concourse.bass_interp import MultiCoreSim

        sim = MultiCoreSim(nc, num_cores=C)
        for k, core_sim in sim.cores.items():
            for name, val in in_maps[k].items():
                core_sim.tensor(name)[:] = val
        sim.simulate()
        y = np.empty((N, OUT), np.float32)
        perm = meta["perm"]
        for k in range(C):
            y[perm[k * NPC : (k + 1) * NPC]] = np.asarray(
                sim.cores[k].tensor("ypad")[:NPC], dtype=np.float32
            )
        return y

    global LAST_NC, LAST_RUN_S
    LAST_NC = nc
    import time as _time

    _t0 = _time.time()
    res = bass_utils.run_bass_kernel_spmd(
        nc,
        in_maps,
        core_ids=list(range(C)),
        trace=cfg.get("TRACE", False),
    )
    LAST_RUN_S = _time.time() - _t0
    LAST_RESULTS = res

    y = np.empty((N, OUT), np.float32)
    perm = meta["perm"]
    for k in range(C):
        y[perm[k * NPC : (k + 1) * NPC]] = np.asarray(
            res.results[k]["ypad"][:NPC], dtype=np.float32
        )
    return y


# revision 14
# speedup vs baseline: 1.4062x; 1.0117x over previous
"""GCN block (GraphConv + BatchNorm1d + ReLU) on 8 Trainium2 NeuronCores.

Strategy — "gather x, apply W after aggregation":

By linearity, agg[dst] = sum_e norm_src[src_e] * (x[src_e] @ W)
                       = (sum_e norm_src[src_e] * x[src_e]) @ W.
So instead of computing h = x@W per shard and AllGather-ing the h table
across cores (collectives dominated the v1 timeline), every core receives
the FULL x (bf16, row-major) in its own HBM and directly dma_gathers raw x
rows for its edges. No AllGather of features at all, and gathers start at
t=0. x rows are 256 bf16 = 512 B, which also clears the <512 B
small-descriptor DMA penalty that h rows (128 bf16 = 256 B) would pay.

Per core k (owns a permuted 1/8 of the dst nodes):
  1. For each 128-edge block (edges bucketed by (src-bank, dst-group)),
     gather x[src] rows (batched dma_gather, bf16) and segment-sum them with
     one-hot matmuls Mw^T @ G accumulated in PSUM (4 groups in flight, one
     PSUM bank per open accumulation chain). The mask is built on DVE as
     (iota == dstoff) * w_e with w_e = rsqrt(deg_out[src_e]) — norm_src is
     folded into the aggregation.
  2. Per finished dst group: agg [128,256] PSUM -> bf16 SBUF -> PE transpose
     (into a bf16-bitcast tail of the group's own agg bank) -> aggT_j^T @ W_j
     -> out [128, OUT] PSUM; relu(out * rsqrt(clip(deg_in,1)) + b) -> bf16;
     BN partial sums via ones-matmuls into a single PSUM stats chain.
  3. AllGather the [1, 2*OUT] per-core BN sums (cheaper than AllReduce),
     reduce with a ones8 matmul, build the affine S/T tiles, then
     y = hrelu * S + T in-place (S/T broadcast along the group axis, split
     across DVE and GpSimd) with the bf16 output DMA chunk-interleaved.
     The host upcasts y to fp32 (quantization ~0.4% rms << 2e-2 gate).

Host-side work is limited to integer index bookkeeping (edge bucketing,
degree counting, node permutation) and layout transforms (bf16 casts, int16
gather indices). All floating-point math runs on device.

Edges are bucketed by src bank (4 banks of N/4 rows) because dma_gather
indices are int16 (< 32768). Bucket capacities are the exact max over the 8
cores (shared SPMD structure; one NEFF serves every core); dst nodes are
permuted so cores\' same-position buckets have near-identical per-bank edge
counts (lexsort by per-bank degree vector, dealt round-robin), which cuts
the max-over-cores padding to ~3.6%. 128-edge blocks may straddle bucket
boundaries -> one matmul per (block, overlapped group). Pad slots gather
row 0 and carry dst offset 255 -> their one-hot column is all zeros, so
they contribute exactly 0. Banks 0 and 3 get >= 1 slot per group so every
group\'s accumulation chain starts in bank 0 and stops in bank 3, keeping
the BN-stat chain\'s start first / stop last in program order.
"""
import math
import os
import sys

sys.path.insert(0, "/opt/trn_rl_repo")

import numpy as np

import concourse.bacc as bacc
import concourse.bass as bass
import concourse.mybir as mybir
import concourse.tile as tile
from concourse import bass_utils

F32 = mybir.dt.float32
BF16 = mybir.dt.bfloat16
I16 = mybir.dt.int16

CFG = dict(
    N=100000,
    E=1600000,
    IN=256,
    OUT=128,
    NCORES=8,
    GRP=128,          # dst nodes per segment group (= psum partition dim)
    NBANKS=4,         # src banks (bank rows must stay < 32768 for int16 idx)
    GCHUNK=4,         # dst groups in flight (1 PSUM bank per open accum chain)
    BATCH_BLOCKS=17,  # gather batch size in 128-edge blocks
    EPS=1e-5,
    TRACE=False,
)

LAST_RESULTS = None  # set by kernel() for test harness introspection
LAST_NC = None
LAST_RUN_S = None


def _ceil_div(a, b):
    return (a + b - 1) // b


def _wrap16(idx, ncols):
    """int16 idx list -> [128, ncols] tile: idx i at [i%16, i//16], replicated
    8x across the 16-partition groups (one copy per GpSimd Q7 core)."""
    n = idx.shape[0]
    assert n == ncols * 16
    w = np.ascontiguousarray(idx.reshape(ncols, 16).T)
    return np.tile(w, (8, 1))


def _preprocess(cfg, src, dst):
    """Bucket edges by (owner core, src bank, dst group); build per-core
    gather-index / dst-offset / src-degree arrays and the shared block
    structure."""
    N, E = cfg["N"], cfg["E"]
    C, NBANKS, GRP = cfg["NCORES"], cfg["NBANKS"], cfg["GRP"]
    NPC = N // C
    NG = _ceil_div(NPC, GRP)
    assert N % NBANKS == 0
    BROWS = N // NBANKS            # rows per x bank (gather source table)
    assert BROWS < 32768

    src = src.astype(np.int64)
    dst = dst.astype(np.int64)
    deg_out = np.bincount(src, minlength=N).astype(np.float32)
    deg_in = np.bincount(dst, minlength=N).astype(np.float32)
    bank = src // BROWS

    # permute dst nodes into (core, group, slot) positions so that the 8
    # cores' group-i buckets have near-identical per-bank in-edge counts --
    # shrinks the SPMD max-over-cores padding of the shared block structure.
    # Nodes sorted by per-bank count vector are dealt round-robin across
    # cores at each (group, slot).
    nbc = np.zeros((N, NBANKS), np.int64)
    for b in range(NBANKS):
        nbc[:, b] = np.bincount(dst[bank == b], minlength=N)
    node_order = np.lexsort(tuple(-nbc[:, b] for b in range(NBANKS - 1, -1, -1)))
    allpos = np.arange(N)
    deal_order = np.lexsort((allpos // NPC, allpos % GRP, (allpos % NPC) // GRP))
    perm = np.empty(N, np.int64)      # new position -> original node
    perm[deal_order] = node_order
    pos = np.empty(N, np.int64)       # original node -> new position
    pos[perm] = allpos

    pdst = pos[dst]
    owner = pdst // NPC
    grp = (pdst % NPC) // GRP
    key = (owner * NBANKS + bank) * NG + grp
    order = np.argsort(key, kind="stable")
    s_src = src[order]
    s_pdst = pos[dst[order]]
    s_key = key[order]

    counts = np.bincount(key, minlength=C * NBANKS * NG).reshape(C, NBANKS, NG)
    # bucket capacity: exact max over cores (shared SPMD structure); >= 1 in
    # banks 0/3 so every group has a first (bank-0) and last (bank-3) matmul
    P = counts.max(axis=0)  # [NBANKS, NG]
    P[0] = np.maximum(P[0], 1)
    P[NBANKS - 1] = np.maximum(P[NBANKS - 1], 1)

    # stream order: group-chunks outer, banks inner; a group's PSUM slot is
    # live across all banks of its chunk (accumulated with start/stop).
    # Segment (chunk, bank) = that chunk's buckets concatenated, padded to a
    # multiple of 128; 128-edge blocks may straddle bucket (group) boundaries
    # -> one matmul per (block, overlapped group).
    GC = cfg["GCHUNK"]
    chunks = [list(range(c, min(c + GC, NG))) for c in range(0, NG, GC)]
    run_off = np.zeros((NBANKS, NG), np.int64)
    segments = []  # (bank, seg_start_slot, seg_nblocks)
    pos = 0
    for ch in chunks:
        for b in range(NBANKS):
            seg0 = pos
            for g in ch:
                run_off[b, g] = pos
                pos += P[b, g]
            pos = ((pos + 127) // 128) * 128  # segment tail pad
            segments.append((b, seg0, (pos - seg0) // 128))
    nidx_tot = pos
    nb_tot = nidx_tot // 128

    # per block: list of matmuls (mm_col, group); per group: first/last mm id
    block_mms = [[] for _ in range(nb_tot)]
    n_mm = 0
    mm_of_group = {}
    for ch in chunks:
        for b in range(NBANKS):
            for g in ch:
                o0, o1 = run_off[b, g], run_off[b, g] + P[b, g]
                for t in range(o0 // 128, (o1 - 1) // 128 + 1):
                    block_mms[t].append((n_mm, g))
                    mm_of_group.setdefault(g, []).append(n_mm)
                    n_mm += 1
    mm_flags = {}
    for g, mms in mm_of_group.items():
        for m in mms:
            mm_flags[m] = (m == mms[0], m == mms[-1])

    # boundaries of each (k, b, g) bucket in the sorted edge stream
    bkeys = (np.arange(C)[:, None, None] * NBANKS + np.arange(NBANKS)[None, :, None]) * NG + np.arange(NG)[None, None, :]
    starts = np.searchsorted(s_key, bkeys.ravel()).reshape(C, NBANKS, NG)
    ends = np.searchsorted(s_key, bkeys.ravel(), side="right").reshape(C, NBANKS, NG)

    gidx_cores = []
    dstoff_cores = []
    wdeg_cores = []
    for k in range(C):
        gidx = np.zeros(nidx_tot, np.int16)
        doff = np.full((n_mm, 128), 255.0, np.float32)
        wdeg = np.ones((n_mm, 128), np.float32)
        # fill gather indices per bucket (pad slots keep row 0)
        for b in range(NBANKS):
            for g in range(NG):
                s, e = starts[k, b, g], ends[k, b, g]
                cnt = e - s
                if cnt == 0:
                    continue
                p0 = run_off[b, g]
                gidx[p0 : p0 + cnt] = (s_src[s:e] % BROWS).astype(np.int16)
        # fill per-matmul mask columns: rows = this core's real edges of the
        # matmul's group that fall inside the block's 128-slot window
        for ch in chunks:
            for b in range(NBANKS):
                for g in ch:
                    s, e = starts[k, b, g], ends[k, b, g]
                    cnt = e - s
                    o0 = run_off[b, g]
                    bucket_mms = [
                        (m, t)
                        for t in range(o0 // 128, (o0 + P[b, g] - 1) // 128 + 1)
                        for (m, gg) in block_mms[t]
                        if gg == g
                    ]
                    if cnt == 0:
                        continue
                    dvals = ((s_pdst[s:e] % NPC) - g * GRP).astype(np.float32)
                    wvals = deg_out[s_src[s:e]]
                    for m, t in bucket_mms:
                        w0 = t * 128
                        lo = max(o0, w0)
                        hi = min(o0 + cnt, w0 + 128)
                        if lo >= hi:
                            continue
                        rows = np.arange(lo - w0, hi - w0)
                        doff[m, rows] = dvals[lo - o0 : hi - o0]
                        wdeg[m, rows] = wvals[lo - o0 : hi - o0]
        gidx_cores.append(_wrap16(gidx, nidx_tot // 16))
        dstoff_cores.append(np.ascontiguousarray(doff.T))
        wdeg_cores.append(np.ascontiguousarray(wdeg.T))

    # gather batches: split segments longer than BATCH_BLOCKS
    batches = []  # (bank, first_block, n_blocks)
    for b, seg0, nblk in segments:
        t = seg0 // 128
        left = nblk
        while left > 0:
            n = min(left, cfg["BATCH_BLOCKS"])
            batches.append((b, t, n))
            t += n
            left -= n

    meta = dict(
        NPC=NPC,
        NG=NG,
        BROWS=BROWS,
        nidx_tot=nidx_tot,
        nb_tot=nb_tot,
        n_mm=n_mm,
        block_mms=block_mms,
        mm_flags=mm_flags,
        batches=batches,
        deg_in=deg_in[perm],
        perm=perm,
    )
    return meta, gidx_cores, dstoff_cores, wdeg_cores


def _tile_major(vec, NG, GRP, pad_val):
    """[NPC] -> [GRP, NG]: entry (p, m) = vec[m*GRP + p], padded."""
    out = np.full((NG * GRP,), pad_val, vec.dtype)
    out[: vec.shape[0]] = vec
    return np.ascontiguousarray(out.reshape(NG, GRP).T)


def _build_nc(cfg, meta):
    N, IN, OUT, C = cfg["N"], cfg["IN"], cfg["OUT"], cfg["NCORES"]
    GRP, NBANKS, GC = cfg["GRP"], cfg["NBANKS"], cfg["GCHUNK"]
    NPC, NG, BROWS = meta["NPC"], meta["NG"], meta["BROWS"]
    nidx_tot, nb_tot = meta["nidx_tot"], meta["nb_tot"]
    n_mm = meta["n_mm"]
    block_mms, mm_flags = meta["block_mms"], meta["mm_flags"]
    batches = meta["batches"]
    XK = _ceil_div(IN, 128)
    assert OUT == 128 and GRP == 128 and XK == 2
    last_w = NPC - (NG - 1) * GRP  # valid rows in the last group

    nc = bacc.Bacc(
        "TRN2", target_bir_lowering=False, debug=False, num_devices=C
    )

    # ---- external inputs ----
    xb = [
        nc.dram_tensor(f"xb{q}", [BROWS, IN], BF16, kind="ExternalInput")
        for q in range(NBANKS)
    ]
    wt = [
        nc.dram_tensor(f"wt{j}", [128, OUT], BF16, kind="ExternalInput")
        for j in range(XK)
    ]
    gidx_d = nc.dram_tensor("gidx", [128, nidx_tot // 16], I16, kind="ExternalInput")
    doff_d = nc.dram_tensor("doff", [128, n_mm], F32, kind="ExternalInput")
    wdeg_d = nc.dram_tensor("wdeg", [128, n_mm], F32, kind="ExternalInput")
    degi_d = nc.dram_tensor("degi", [128, NG], F32, kind="ExternalInput")
    bt_d = nc.dram_tensor("bt", [128, OUT], F32, kind="ExternalInput")
    iota_d = nc.dram_tensor("iota", [128, GRP], BF16, kind="ExternalInput")
    ident_d = nc.dram_tensor("ident", [128, 128], BF16, kind="ExternalInput")
    gm_d = nc.dram_tensor("gm", [1, OUT], F32, kind="ExternalInput")
    bb_d = nc.dram_tensor("bb", [1, OUT], F32, kind="ExternalInput")
    onesc_d = nc.dram_tensor("onesc", [128, 1], BF16, kind="ExternalInput")
    ones8_d = nc.dram_tensor("ones8", [8, 1], F32, kind="ExternalInput")
    onest_d = nc.dram_tensor("onest", [128, 1], BF16, kind="ExternalInput")
    onesr_d = nc.dram_tensor("onesr", [1, 128], F32, kind="ExternalInput")

    ypad_d = nc.dram_tensor("ypad", [NG * GRP, OUT], BF16, kind="ExternalOutput")

    with tile.TileContext(nc) as tc:
        with (
            tc.tile_pool(name="const", bufs=1) as cpool,
            tc.tile_pool(name="dram", bufs=1, space="DRAM") as dpool,
            tc.tile_pool(name="hrelu", bufs=1) as hpool,
            tc.tile_pool(name="gath", bufs=7) as gpool,
            tc.tile_pool(name="mpool", bufs=10) as mpool,
            tc.tile_pool(name="capool", bufs=6) as capool,
            tc.tile_pool(name="etmp", bufs=6) as epool,
            tc.tile_pool(name="gtmp", bufs=4) as gpool2,
            tc.tile_pool(name="psagg", bufs=1, space="PSUM") as pagg,
            tc.tile_pool(name="psaux", bufs=1, space="PSUM") as paux,
            tc.tile_pool(name="pstat", bufs=1, space="PSUM") as pspool,
        ):
            # ---- constants / small tiles ----
            iota_t = cpool.tile([128, GRP], BF16)
            ident_t = cpool.tile([128, 128], BF16)
            bt_t = cpool.tile([128, OUT], F32)
            degi_t = cpool.tile([128, NG], F32)
            ndst_t = cpool.tile([128, NG], F32)
            gm_t = cpool.tile([1, OUT], F32)
            bb_t = cpool.tile([1, OUT], F32)
            onesc_t = cpool.tile([128, 1], BF16)
            ones8_t = cpool.tile([8, 1], F32)
            onest_t = cpool.tile([128, 1], BF16)
            onesr_t = cpool.tile([1, 128], F32)
            gidx_t = cpool.tile([128, nidx_tot // 16], I16)
            doff_t = cpool.tile([128, n_mm], F32)
            wsrc_t = cpool.tile([128, n_mm], F32)
            wts = [cpool.tile([128, OUT], BF16, name=f"wt_s{j}") for j in range(XK)]

            # split the big index loads so the first gathers aren't gated on
            # the full-table DMA
            gsplit = min(nidx_tot // 16, 256)
            nc.sync.dma_start(gidx_t[:, :gsplit], gidx_d[:, :gsplit])
            if gsplit < nidx_tot // 16:
                nc.sync.dma_start(gidx_t[:, gsplit:], gidx_d[:, gsplit:])
            dsplit = min(n_mm, 128)
            nc.sync.dma_start(doff_t[:, :dsplit], doff_d[:, :dsplit])
            nc.sync.dma_start(wsrc_t[:, :dsplit], wdeg_d[:, :dsplit])
            if dsplit < n_mm:
                nc.sync.dma_start(doff_t[:, dsplit:], doff_d[:, dsplit:])
                nc.sync.dma_start(wsrc_t[:, dsplit:], wdeg_d[:, dsplit:])
            nc.sync.dma_start(iota_t[:], iota_d[:])
            nc.sync.dma_start(ident_t[:], ident_d[:])
            nc.sync.dma_start(bt_t[:], bt_d[:])
            nc.sync.dma_start(degi_t[:], degi_d[:])
            nc.sync.dma_start(gm_t[:], gm_d[:])
            nc.sync.dma_start(bb_t[:], bb_d[:])
            nc.sync.dma_start(onesc_t[:], onesc_d[:])
            nc.sync.dma_start(ones8_t[:], ones8_d[:])
            nc.sync.dma_start(onest_t[:], onest_d[:])
            nc.sync.dma_start(onesr_t[:], onesr_d[:])
            for j in range(XK):
                nc.sync.dma_start(wts[j][:], wt[j][:])

            # per-edge src norm: w = rsqrt(deg_out[src]) (pad slots carry 1.0);
            # two pieces so the first masks aren't gated on the full tile
            for c0, c1 in ((0, dsplit), (dsplit, n_mm)):
                if c0 >= c1:
                    continue
                nc.vector.reciprocal(wsrc_t[:, c0:c1], wsrc_t[:, c0:c1])
                nc.scalar.activation(
                    wsrc_t[:, c0:c1], wsrc_t[:, c0:c1],
                    mybir.ActivationFunctionType.Sqrt,
                )
            # dst norm: rsqrt(max(deg_in, 1)) tile-major [GRP, NG]
            nc.vector.tensor_scalar(
                ndst_t[:], degi_t[:], 1.0, None, op0=mybir.AluOpType.max
            )
            nc.vector.reciprocal(ndst_t[:], ndst_t[:])
            nc.scalar.activation(
                ndst_t[:], ndst_t[:], mybir.ActivationFunctionType.Sqrt
            )

            stats_in = dpool.tile([1, 2 * OUT], F32)
            _aspace = "Local" if cfg.get("NOCC") else "Shared"
            stats_out = dpool.tile([C, 2 * OUT], F32, addr_space=_aspace)

            hrelu_t = hpool.tile([128, NG, OUT], BF16)

            # ---- PSUM layout (8 banks x 2KB); accumulation-group zeroing is
            # bank-granular, so every concurrently-open chain gets its own
            # bank: 4x agg (GCHUNK groups in flight) + 1x transpose + 2x out
            # (alternating, WAR-tracked) + 1x BN stats (sum+sq as one chain).
            assert GC in (4, 5)
            # agg tiles carry a 512B tail used (bitcast bf16) as the per-group
            # transpose target, so each group's finish chain owns its own bank
            ps_agg = [
                pagg.tile([128, 2 * OUT + 128], F32, name=f"ps_agg{i}")
                for i in range(GC)
            ]
            n_out = 8 - GC - 1
            ps_out = [
                paux.tile([128, OUT], F32, name=f"ps_out{i}") for i in range(n_out)
            ]
            ps_stat = pspool.tile([1, 2, OUT], F32, name="ps_stat")

            # ---- stage D: gather x rows + one-hot matmul segmented sum ----
            def _finish_group(g):
                """Group g's PSUM agg is complete: apply W, relu, BN partials."""
                gi = g % GC
                cagg = capool.tile([128, 2 * OUT], BF16, tag="cagg")
                nc.scalar.activation(
                    cagg[:], ps_agg[gi][:, : 2 * OUT],
                    mybir.ActivationFunctionType.Copy,
                )
                # transpose into this group's own agg bank tail (bitcast bf16)
                # so concurrent groups' finish chains don't share a PSUM bank
                trv = ps_agg[gi][:, 2 * OUT :].bitcast(BF16)
                for h in range(2):
                    nc.tensor.matmul(
                        trv[:, h * OUT : (h + 1) * OUT],
                        cagg[:, h * OUT : (h + 1) * OUT],
                        ident_t[:],
                        is_transpose=True,
                        start=(h == 0),
                        stop=(h == 1),
                    )
                tagg = capool.tile([128, 2, OUT], BF16, tag="tagg")
                nc.scalar.activation(
                    tagg[:, :, :],
                    trv[:].rearrange("p (a f) -> p a f", a=2),
                    mybir.ActivationFunctionType.Copy,
                )
                po = ps_out[g % len(ps_out)]
                for jj in range(XK):
                    nc.tensor.matmul(
                        po[:],
                        tagg[:, jj, :],
                        wts[jj][:],
                        start=(jj == 0),
                        stop=(jj == XK - 1),
                    )
                # stage E: relu(out * ndst + b), BN partial sums
                tmp = epool.tile([128, OUT], F32, tag="etmp")
                nc.vector.scalar_tensor_tensor(
                    tmp[:],
                    po[:],
                    ndst_t[:, g : g + 1],
                    bt_t[:],
                    op0=mybir.AluOpType.mult,
                    op1=mybir.AluOpType.add,
                )
                nc.scalar.activation(
                    hrelu_t[:, g, :], tmp[:], mybir.ActivationFunctionType.Relu
                )
                ones = onesc_t if g < NG - 1 else onest_t
                nc.tensor.matmul(
                    ps_stat[:, 0, :],
                    ones[:],
                    hrelu_t[:, g, :],
                    start=(g == 0),
                    stop=False,
                )
                sq = epool.tile([128, OUT], BF16, tag="esq")
                nc.scalar.activation(
                    sq[:], hrelu_t[:, g, :], mybir.ActivationFunctionType.Square
                )
                nc.tensor.matmul(
                    ps_stat[:, 1, :],
                    ones[:],
                    sq[:],
                    start=False,
                    stop=(g == NG - 1),
                )

            bmax = max(nb for _, _, nb in batches)
            # finish chains are deferred until the group's PSUM slot is about
            # to be reused, so they sit behind the next chunk's early mask
            # matmuls in the in-order PE queue instead of blocking them
            pending = {}
            for bank, t0, nblk in batches:
                Gt = gpool.tile([128, bmax, IN], BF16, tag="G")
                nc.gpsimd.dma_gather(
                    Gt[:, :nblk, :],
                    xb[bank][:],
                    gidx_t[:, t0 * 8 : (t0 + nblk) * 8],
                    nblk * 128,
                    nblk * 128,
                    IN,
                    single_packet=False,
                )
                for j in range(nblk):
                    t = t0 + j
                    for m, g in block_mms[t]:
                      is_start, is_stop = mm_flags[m]
                      gi = g % GC
                      if is_start and gi in pending:
                          _finish_group(pending.pop(gi))
                      Mt = mpool.tile([128, GRP], BF16, tag="M")
                      nc.vector.tensor_scalar(
                          Mt[:],
                          iota_t[:],
                          doff_t[:, m : m + 1],
                          wsrc_t[:, m : m + 1],
                          op0=mybir.AluOpType.is_equal,
                          op1=mybir.AluOpType.mult,
                      )
                      nc.tensor.matmul(
                          ps_agg[gi][:, : 2 * OUT],
                          Mt[:],
                          Gt[:, j, :],
                          start=is_start,
                          stop=is_stop,
                      )
                      if is_stop:
                          pending[gi] = g

            for gi in sorted(pending):
                _finish_group(pending.pop(gi))

            # ---- stage F: AllReduce BN stats; build affine S/T tiles ----
            S_t = cpool.tile([128, OUT], BF16)
            T_t = cpool.tile([128, OUT], BF16)
            st_sb = cpool.tile([1, 2 * OUT], F32)
            nc.scalar.activation(
                st_sb[:], ps_stat[:].rearrange("p a f -> p (a f)"),
                mybir.ActivationFunctionType.Copy,
            )
            nc.sync.dma_start(stats_in[:], st_sb[:])
            if cfg.get("NOCC"):
                stats_out = stats_in  # single-core debug: sums are the totals
                st8 = st_sb
                ones8v = None
            else:
                # AllGather (no 1.875x reduce multiplier) + tiny local
                # ones-matmul reduction beats AllReduce on latency
                nc.gpsimd.collective_compute(
                    "AllGather",
                    mybir.AluOpType.bypass,
                    replica_groups=[list(range(C))],
                    ins=[stats_in[:]],
                    outs=[stats_out[:]],
                )
                st8 = cpool.tile([C, 2 * OUT], F32)
                nc.sync.dma_start(st8[:], stats_out[:])
                ones8v = ones8_t
            if ones8v is None:
                st_rb = cpool.tile([1, 2 * OUT], F32)
                nc.scalar.activation(
                    st_rb[:], st8[:], mybir.ActivationFunctionType.Copy
                )
                red_v = st_rb[:]
            else:
                ps_red = ps_stat[:].rearrange("p a f -> p (a f)")
                nc.tensor.matmul(ps_red, ones8v[:], st8[:], start=True, stop=True)
                red_v = ps_red

            mu = cpool.tile([1, OUT], F32)
            musq = cpool.tile([1, OUT], F32)
            ex2e = cpool.tile([1, OUT], F32)
            var = cpool.tile([1, OUT], F32)
            srow = cpool.tile([1, OUT], F32)
            trow = cpool.tile([1, OUT], F32)
            inv_n = 1.0 / float(N)
            # mu/ex2 straight off the reduction PSUM; fold 1/N and eps in
            nc.scalar.activation(
                mu[:], red_v[:, 0:OUT], mybir.ActivationFunctionType.Copy, scale=inv_n
            )
            nc.scalar.activation(
                ex2e[:],
                red_v[:, OUT : 2 * OUT],
                mybir.ActivationFunctionType.Copy,
                scale=inv_n,
                bias=float(cfg["EPS"]),
            )
            nc.scalar.activation(
                musq[:], mu[:], mybir.ActivationFunctionType.Square
            )
            # var + eps, then rsqrt via reciprocal+sqrt (ACT Rsqrt is banned
            # for accuracy)
            nc.vector.tensor_sub(var[:], ex2e[:], musq[:])
            nc.vector.reciprocal(var[:], var[:])
            nc.scalar.activation(
                var[:], var[:], mybir.ActivationFunctionType.Sqrt
            )
            nc.vector.tensor_mul(srow[:], gm_t[:], var[:])
            nc.vector.tensor_mul(trow[:], mu[:], srow[:])
            nc.vector.tensor_sub(trow[:], bb_t[:], trow[:])

            # reuse the (now idle) out banks for the S/T broadcast matmuls
            nc.tensor.matmul(ps_out[0][:], onesr_t[:], srow[:], start=True, stop=True)
            nc.scalar.activation(
                S_t[:], ps_out[0][:], mybir.ActivationFunctionType.Copy
            )
            nc.tensor.matmul(ps_out[1][:], onesr_t[:], trow[:], start=True, stop=True)
            nc.scalar.activation(
                T_t[:], ps_out[1][:], mybir.ActivationFunctionType.Copy
            )

            # ---- stage G: y = hrelu * S + T (in place, S/T broadcast along
            # the group axis), output DMA chunked to overlap with the DVE ----
            ypad_view = ypad_d[:].rearrange("(g p) f -> p g f", p=128)
            GOUT = 7
            for ci, c0 in enumerate(range(0, NG, GOUT)):
                c1 = min(c0 + GOUT, NG)
                S_b = S_t[:].rearrange("p (a f) -> p a f", a=1).to_broadcast(
                    (128, c1 - c0, OUT)
                )
                T_b = T_t[:].rearrange("p (a f) -> p a f", a=1).to_broadcast(
                    (128, c1 - c0, OUT)
                )
                eng = nc.gpsimd if ci >= 10 else nc.vector
                eng.tensor_mul(
                    hrelu_t[:, c0:c1, :], hrelu_t[:, c0:c1, :], S_b
                )
                eng.tensor_add(
                    hrelu_t[:, c0:c1, :], hrelu_t[:, c0:c1, :], T_b
                )
                nc.sync.dma_start(
                    ypad_view[:, c0:c1, :], hrelu_t[:, c0:c1, :]
                )

    nc.compile()
    return nc


def kernel(x, src, dst, W, b, gamma, beta):
    global LAST_RESULTS
    cfg = CFG
    N, E, IN, OUT, C = cfg["N"], cfg["E"], cfg["IN"], cfg["OUT"], cfg["NCORES"]
    GRP = cfg["GRP"]
    assert x.shape == (N, IN) and W.shape == (IN, OUT)
    assert src.shape == (E,) and dst.shape == (E,)

    meta, gidx_cores, dstoff_cores, wdeg_cores = _preprocess(cfg, src, dst)
    NPC, NG, BROWS = meta["NPC"], meta["NG"], meta["BROWS"]
    XK = _ceil_div(IN, 128)
    last_w = NPC - (NG - 1) * GRP

    nc = _build_nc(cfg, meta)

    import ml_dtypes

    x_bf = np.asarray(x, np.float32).astype(ml_dtypes.bfloat16)
    Wn = np.asarray(W, np.float32)

    iota = np.tile(
        np.arange(GRP, dtype=np.float32)[None, :], (128, 1)
    ).astype(ml_dtypes.bfloat16)
    ident = np.eye(128, dtype=np.float32).astype(ml_dtypes.bfloat16)
    bt = np.tile(np.asarray(b, np.float32)[None, :], (128, 1))
    onesc = np.ones((128, 1), np.float32).astype(ml_dtypes.bfloat16)
    ones8 = np.ones((8, 1), np.float32)
    onest = np.zeros((128, 1), np.float32)
    onest[:last_w] = 1.0
    onest = onest.astype(ml_dtypes.bfloat16)
    onesr = np.ones((1, 128), np.float32)
    gm = np.asarray(gamma, np.float32)[None, :]
    bb = np.asarray(beta, np.float32)[None, :]
    xbanks = [
        np.ascontiguousarray(x_bf[q * BROWS : (q + 1) * BROWS])
        for q in range(cfg["NBANKS"])
    ]
    wtiles = [
        np.ascontiguousarray(Wn[j * 128 : (j + 1) * 128, :]).astype(
            ml_dtypes.bfloat16
        )
        for j in range(XK)
    ]

    in_maps = []
    for k in range(C):
        im = {
            "gidx": gidx_cores[k],
            "doff": dstoff_cores[k],
            "wdeg": wdeg_cores[k],
            "degi": _tile_major(
                meta["deg_in"][k * NPC : (k + 1) * NPC], NG, GRP, np.float32(1.0)
            ),
            "bt": bt,
            "iota": iota,
            "ident": ident,
            "gm": gm,
            "bb": bb,
            "onesc": onesc,
            "ones8": ones8,
            "onest": onest,
            "onesr": onesr,
        }
        for q in range(cfg["NBANKS"]):
            im[f"xb{q}"] = xbanks[q]
        for j in range(XK):
            im[f"wt{j}"] = wtiles[j]
        in_maps.append(im)

    if cfg.get("SIM"):
        from concourse.bass_interp import MultiCoreSim

        sim = MultiCoreSim(nc, num_cores=C)
        for k, core_sim in sim.cores.items():
            for name, val in in_maps[k].items():
                core_sim.tensor(name)[:] = val
        sim.simulate()
        y = np.empty((N, OUT), np.float32)
        perm = meta["perm"]
        for k in range(C):
            y[perm[k * NPC : (k + 1) * NPC]] = np.asarray(
                sim.cores[k].tensor("ypad")[:NPC], dtype=np.float32
            )
        return y

    global LAST_NC, LAST_RUN_S
    LAST_NC = nc
    import time as _time

    _t0 = _time.time()
    res = bass_utils.run_bass_kernel_spmd(
        nc,
        in_maps,
        core_ids=list(range(C)),
        trace=cfg.get("TRACE", False),
    )
    LAST_RUN_S = _time.time() - _t0
    LAST_RESULTS = res

    y = np.empty((N, OUT), np.float32)
    perm = meta["perm"]
    for k in range(C):
        y[perm[k * NPC : (k + 1) * NPC]] = np.asarray(
            res.results[k]["ypad"][:NPC], dtype=np.float32
        )
    return y


# revision 15
# speedup vs baseline: 1.4094x; 1.0023x over previous
"""GCN block (GraphConv + BatchNorm1d + ReLU) on 8 Trainium2 NeuronCores.

Strategy — "gather x, apply W after aggregation":

By linearity, agg[dst] = sum_e norm_src[src_e] * (x[src_e] @ W)
                       = (sum_e norm_src[src_e] * x[src_e]) @ W.
So instead of computing h = x@W per shard and AllGather-ing the h table
across cores (collectives dominated the v1 timeline), every core receives
the FULL x (bf16, row-major) in its own HBM and directly dma_gathers raw x
rows for its edges. No AllGather of features at all, and gathers start at
t=0. x rows are 256 bf16 = 512 B, which also clears the <512 B
small-descriptor DMA penalty that h rows (128 bf16 = 256 B) would pay.

Per core k (owns a permuted 1/8 of the dst nodes):
  1. For each 128-edge block (edges bucketed by (src-bank, dst-group)),
     gather x[src] rows (batched dma_gather, bf16) and segment-sum them with
     one-hot matmuls Mw^T @ G accumulated in PSUM (4 groups in flight, one
     PSUM bank per open accumulation chain). The mask is built on DVE as
     (iota == dstoff) * w_e with w_e = rsqrt(deg_out[src_e]) — norm_src is
     folded into the aggregation.
  2. Per finished dst group: agg [128,256] PSUM -> bf16 SBUF -> PE transpose
     (into a bf16-bitcast tail of the group's own agg bank) -> aggT_j^T @ W_j
     -> out [128, OUT] PSUM; relu(out * rsqrt(clip(deg_in,1)) + b) -> bf16;
     BN partial sums via ones-matmuls into a single PSUM stats chain.
  3. AllGather the [1, 2*OUT] per-core BN sums (cheaper than AllReduce),
     reduce with a ones8 matmul, build the affine S/T tiles, then
     y = hrelu * S + T in-place (S/T broadcast along the group axis, split
     across DVE and GpSimd) with the bf16 output DMA chunk-interleaved.
     The host upcasts y to fp32 (quantization ~0.4% rms << 2e-2 gate).

Host-side work is limited to integer index bookkeeping (edge bucketing,
degree counting, node permutation) and layout transforms (bf16 casts, int16
gather indices). All floating-point math runs on device.

Edges are bucketed by src bank (4 banks of N/4 rows) because dma_gather
indices are int16 (< 32768). Bucket capacities are the exact max over the 8
cores (shared SPMD structure; one NEFF serves every core); dst nodes are
permuted so cores\' same-position buckets have near-identical per-bank edge
counts (lexsort by per-bank degree vector, dealt round-robin), which cuts
the max-over-cores padding to ~3.6%. 128-edge blocks may straddle bucket
boundaries -> one matmul per (block, overlapped group). Pad slots gather
row 0 and carry dst offset 255 -> their one-hot column is all zeros, so
they contribute exactly 0. Banks 0 and 3 get >= 1 slot per group so every
group\'s accumulation chain starts in bank 0 and stops in bank 3, keeping
the BN-stat chain\'s start first / stop last in program order.
"""
import math
import os
import sys

sys.path.insert(0, "/opt/trn_rl_repo")

import numpy as np

import concourse.bacc as bacc
import concourse.bass as bass
import concourse.mybir as mybir
import concourse.tile as tile
from concourse import bass_utils

F32 = mybir.dt.float32
BF16 = mybir.dt.bfloat16
I16 = mybir.dt.int16

CFG = dict(
    N=100000,
    E=1600000,
    IN=256,
    OUT=128,
    NCORES=8,
    GRP=128,          # dst nodes per segment group (= psum partition dim)
    NBANKS=4,         # src banks (bank rows must stay < 32768 for int16 idx)
    GCHUNK=4,         # dst groups in flight (1 PSUM bank per open accum chain)
    BATCH_BLOCKS=17,  # gather batch size in 128-edge blocks
    EPS=1e-5,
    TRACE=False,
)

LAST_RESULTS = None  # set by kernel() for test harness introspection
LAST_NC = None
LAST_RUN_S = None


def _ceil_div(a, b):
    return (a + b - 1) // b


def _wrap16(idx, ncols):
    """int16 idx list -> [128, ncols] tile: idx i at [i%16, i//16], replicated
    8x across the 16-partition groups (one copy per GpSimd Q7 core)."""
    n = idx.shape[0]
    assert n == ncols * 16
    w = np.ascontiguousarray(idx.reshape(ncols, 16).T)
    return np.tile(w, (8, 1))


def _preprocess(cfg, src, dst):
    """Bucket edges by (owner core, src bank, dst group); build per-core
    gather-index / dst-offset / src-degree arrays and the shared block
    structure."""
    N, E = cfg["N"], cfg["E"]
    C, NBANKS, GRP = cfg["NCORES"], cfg["NBANKS"], cfg["GRP"]
    NPC = N // C
    NG = _ceil_div(NPC, GRP)
    assert N % NBANKS == 0
    BROWS = N // NBANKS            # rows per x bank (gather source table)
    assert BROWS < 32768

    src = src.astype(np.int64)
    dst = dst.astype(np.int64)
    deg_out = np.bincount(src, minlength=N).astype(np.float32)
    deg_in = np.bincount(dst, minlength=N).astype(np.float32)
    bank = src // BROWS

    # permute dst nodes into (core, group, slot) positions so that the 8
    # cores' group-i buckets have near-identical per-bank in-edge counts --
    # shrinks the SPMD max-over-cores padding of the shared block structure.
    # Nodes sorted by per-bank count vector are dealt round-robin across
    # cores at each (group, slot).
    nbc = np.zeros((N, NBANKS), np.int64)
    for b in range(NBANKS):
        nbc[:, b] = np.bincount(dst[bank == b], minlength=N)
    node_order = np.lexsort(tuple(-nbc[:, b] for b in range(NBANKS - 1, -1, -1)))
    allpos = np.arange(N)
    deal_order = np.lexsort((allpos // NPC, allpos % GRP, (allpos % NPC) // GRP))
    perm = np.empty(N, np.int64)      # new position -> original node
    perm[deal_order] = node_order
    pos = np.empty(N, np.int64)       # original node -> new position
    pos[perm] = allpos

    pdst = pos[dst]
    owner = pdst // NPC
    grp = (pdst % NPC) // GRP
    key = (owner * NBANKS + bank) * NG + grp
    order = np.argsort(key, kind="stable")
    s_src = src[order]
    s_pdst = pos[dst[order]]
    s_key = key[order]

    counts = np.bincount(key, minlength=C * NBANKS * NG).reshape(C, NBANKS, NG)
    # bucket capacity: exact max over cores (shared SPMD structure); >= 1 in
    # banks 0/3 so every group has a first (bank-0) and last (bank-3) matmul
    P = counts.max(axis=0)  # [NBANKS, NG]
    P[0] = np.maximum(P[0], 1)
    P[NBANKS - 1] = np.maximum(P[NBANKS - 1], 1)

    # stream order: group-chunks outer, banks inner; a group's PSUM slot is
    # live across all banks of its chunk (accumulated with start/stop).
    # Segment (chunk, bank) = that chunk's buckets concatenated, padded to a
    # multiple of 128; 128-edge blocks may straddle bucket (group) boundaries
    # -> one matmul per (block, overlapped group).
    GC = cfg["GCHUNK"]
    chunks = [list(range(c, min(c + GC, NG))) for c in range(0, NG, GC)]
    run_off = np.zeros((NBANKS, NG), np.int64)
    segments = []  # (bank, seg_start_slot, seg_nblocks)
    pos = 0
    for ch in chunks:
        for b in range(NBANKS):
            seg0 = pos
            for g in ch:
                run_off[b, g] = pos
                pos += P[b, g]
            pos = ((pos + 127) // 128) * 128  # segment tail pad
            segments.append((b, seg0, (pos - seg0) // 128))
    nidx_tot = pos
    nb_tot = nidx_tot // 128

    # per block: list of matmuls (mm_col, group); per group: first/last mm id
    block_mms = [[] for _ in range(nb_tot)]
    n_mm = 0
    mm_of_group = {}
    for ch in chunks:
        for b in range(NBANKS):
            for g in ch:
                o0, o1 = run_off[b, g], run_off[b, g] + P[b, g]
                for t in range(o0 // 128, (o1 - 1) // 128 + 1):
                    block_mms[t].append((n_mm, g))
                    mm_of_group.setdefault(g, []).append(n_mm)
                    n_mm += 1
    mm_flags = {}
    for g, mms in mm_of_group.items():
        for m in mms:
            mm_flags[m] = (m == mms[0], m == mms[-1])

    # boundaries of each (k, b, g) bucket in the sorted edge stream
    bkeys = (np.arange(C)[:, None, None] * NBANKS + np.arange(NBANKS)[None, :, None]) * NG + np.arange(NG)[None, None, :]
    starts = np.searchsorted(s_key, bkeys.ravel()).reshape(C, NBANKS, NG)
    ends = np.searchsorted(s_key, bkeys.ravel(), side="right").reshape(C, NBANKS, NG)

    gidx_cores = []
    dstoff_cores = []
    wdeg_cores = []
    for k in range(C):
        gidx = np.zeros(nidx_tot, np.int16)
        doff = np.full((n_mm, 128), 255.0, np.float32)
        wdeg = np.ones((n_mm, 128), np.float32)
        # fill gather indices per bucket (pad slots keep row 0)
        for b in range(NBANKS):
            for g in range(NG):
                s, e = starts[k, b, g], ends[k, b, g]
                cnt = e - s
                if cnt == 0:
                    continue
                p0 = run_off[b, g]
                gidx[p0 : p0 + cnt] = (s_src[s:e] % BROWS).astype(np.int16)
        # fill per-matmul mask columns: rows = this core's real edges of the
        # matmul's group that fall inside the block's 128-slot window
        for ch in chunks:
            for b in range(NBANKS):
                for g in ch:
                    s, e = starts[k, b, g], ends[k, b, g]
                    cnt = e - s
                    o0 = run_off[b, g]
                    bucket_mms = [
                        (m, t)
                        for t in range(o0 // 128, (o0 + P[b, g] - 1) // 128 + 1)
                        for (m, gg) in block_mms[t]
                        if gg == g
                    ]
                    if cnt == 0:
                        continue
                    dvals = ((s_pdst[s:e] % NPC) - g * GRP).astype(np.float32)
                    wvals = deg_out[s_src[s:e]]
                    for m, t in bucket_mms:
                        w0 = t * 128
                        lo = max(o0, w0)
                        hi = min(o0 + cnt, w0 + 128)
                        if lo >= hi:
                            continue
                        rows = np.arange(lo - w0, hi - w0)
                        doff[m, rows] = dvals[lo - o0 : hi - o0]
                        wdeg[m, rows] = wvals[lo - o0 : hi - o0]
        gidx_cores.append(_wrap16(gidx, nidx_tot // 16))
        dstoff_cores.append(np.ascontiguousarray(doff.T))
        wdeg_cores.append(np.ascontiguousarray(wdeg.T))

    # gather batches: split segments longer than BATCH_BLOCKS
    batches = []  # (bank, first_block, n_blocks)
    for b, seg0, nblk in segments:
        t = seg0 // 128
        left = nblk
        while left > 0:
            n = min(left, cfg["BATCH_BLOCKS"])
            batches.append((b, t, n))
            t += n
            left -= n

    meta = dict(
        NPC=NPC,
        NG=NG,
        BROWS=BROWS,
        nidx_tot=nidx_tot,
        nb_tot=nb_tot,
        n_mm=n_mm,
        block_mms=block_mms,
        mm_flags=mm_flags,
        batches=batches,
        deg_in=deg_in[perm],
        perm=perm,
    )
    return meta, gidx_cores, dstoff_cores, wdeg_cores


def _tile_major(vec, NG, GRP, pad_val):
    """[NPC] -> [GRP, NG]: entry (p, m) = vec[m*GRP + p], padded."""
    out = np.full((NG * GRP,), pad_val, vec.dtype)
    out[: vec.shape[0]] = vec
    return np.ascontiguousarray(out.reshape(NG, GRP).T)


def _build_nc(cfg, meta):
    N, IN, OUT, C = cfg["N"], cfg["IN"], cfg["OUT"], cfg["NCORES"]
    GRP, NBANKS, GC = cfg["GRP"], cfg["NBANKS"], cfg["GCHUNK"]
    NPC, NG, BROWS = meta["NPC"], meta["NG"], meta["BROWS"]
    nidx_tot, nb_tot = meta["nidx_tot"], meta["nb_tot"]
    n_mm = meta["n_mm"]
    block_mms, mm_flags = meta["block_mms"], meta["mm_flags"]
    batches = meta["batches"]
    XK = _ceil_div(IN, 128)
    assert OUT == 128 and GRP == 128 and XK == 2
    last_w = NPC - (NG - 1) * GRP  # valid rows in the last group

    nc = bacc.Bacc(
        "TRN2", target_bir_lowering=False, debug=False, num_devices=C
    )

    # ---- external inputs ----
    xb = [
        nc.dram_tensor(f"xb{q}", [BROWS, IN], BF16, kind="ExternalInput")
        for q in range(NBANKS)
    ]
    wt = [
        nc.dram_tensor(f"wt{j}", [128, OUT], BF16, kind="ExternalInput")
        for j in range(XK)
    ]
    gidx_d = nc.dram_tensor("gidx", [128, nidx_tot // 16], I16, kind="ExternalInput")
    doff_d = nc.dram_tensor("doff", [128, n_mm], F32, kind="ExternalInput")
    wdeg_d = nc.dram_tensor("wdeg", [128, n_mm], F32, kind="ExternalInput")
    degi_d = nc.dram_tensor("degi", [128, NG], F32, kind="ExternalInput")
    bt_d = nc.dram_tensor("bt", [128, OUT], F32, kind="ExternalInput")
    iota_d = nc.dram_tensor("iota", [128, GRP], BF16, kind="ExternalInput")
    ident_d = nc.dram_tensor("ident", [128, 128], BF16, kind="ExternalInput")
    gm_d = nc.dram_tensor("gm", [1, OUT], F32, kind="ExternalInput")
    bb_d = nc.dram_tensor("bb", [1, OUT], F32, kind="ExternalInput")
    onesc_d = nc.dram_tensor("onesc", [128, 1], BF16, kind="ExternalInput")
    ones8_d = nc.dram_tensor("ones8", [8, 1], F32, kind="ExternalInput")
    onest_d = nc.dram_tensor("onest", [128, 1], BF16, kind="ExternalInput")
    onesr_d = nc.dram_tensor("onesr", [1, 128], F32, kind="ExternalInput")

    ypad_d = nc.dram_tensor("ypad", [128, NG * OUT], BF16, kind="ExternalOutput")

    with tile.TileContext(nc) as tc:
        with (
            tc.tile_pool(name="const", bufs=1) as cpool,
            tc.tile_pool(name="dram", bufs=1, space="DRAM") as dpool,
            tc.tile_pool(name="hrelu", bufs=1) as hpool,
            tc.tile_pool(name="gath", bufs=7) as gpool,
            tc.tile_pool(name="mpool", bufs=10) as mpool,
            tc.tile_pool(name="capool", bufs=6) as capool,
            tc.tile_pool(name="etmp", bufs=6) as epool,
            tc.tile_pool(name="gtmp", bufs=4) as gpool2,
            tc.tile_pool(name="psagg", bufs=1, space="PSUM") as pagg,
            tc.tile_pool(name="psaux", bufs=1, space="PSUM") as paux,
            tc.tile_pool(name="pstat", bufs=1, space="PSUM") as pspool,
        ):
            # ---- constants / small tiles ----
            iota_t = cpool.tile([128, GRP], BF16)
            ident_t = cpool.tile([128, 128], BF16)
            bt_t = cpool.tile([128, OUT], F32)
            degi_t = cpool.tile([128, NG], F32)
            ndst_t = cpool.tile([128, NG], F32)
            gm_t = cpool.tile([1, OUT], F32)
            bb_t = cpool.tile([1, OUT], F32)
            onesc_t = cpool.tile([128, 1], BF16)
            ones8_t = cpool.tile([8, 1], F32)
            onest_t = cpool.tile([128, 1], BF16)
            onesr_t = cpool.tile([1, 128], F32)
            gidx_t = cpool.tile([128, nidx_tot // 16], I16)
            doff_t = cpool.tile([128, n_mm], F32)
            wsrc_t = cpool.tile([128, n_mm], F32)
            wts = [cpool.tile([128, OUT], BF16, name=f"wt_s{j}") for j in range(XK)]

            # split the big index loads so the first gathers aren't gated on
            # the full-table DMA
            gsplit = min(nidx_tot // 16, 256)
            nc.sync.dma_start(gidx_t[:, :gsplit], gidx_d[:, :gsplit])
            if gsplit < nidx_tot // 16:
                nc.sync.dma_start(gidx_t[:, gsplit:], gidx_d[:, gsplit:])
            dsplit = min(n_mm, 128)
            nc.sync.dma_start(doff_t[:, :dsplit], doff_d[:, :dsplit])
            nc.sync.dma_start(wsrc_t[:, :dsplit], wdeg_d[:, :dsplit])
            if dsplit < n_mm:
                nc.sync.dma_start(doff_t[:, dsplit:], doff_d[:, dsplit:])
                nc.sync.dma_start(wsrc_t[:, dsplit:], wdeg_d[:, dsplit:])
            nc.sync.dma_start(iota_t[:], iota_d[:])
            nc.sync.dma_start(ident_t[:], ident_d[:])
            nc.sync.dma_start(bt_t[:], bt_d[:])
            nc.sync.dma_start(degi_t[:], degi_d[:])
            nc.sync.dma_start(gm_t[:], gm_d[:])
            nc.sync.dma_start(bb_t[:], bb_d[:])
            nc.sync.dma_start(onesc_t[:], onesc_d[:])
            nc.sync.dma_start(ones8_t[:], ones8_d[:])
            nc.sync.dma_start(onest_t[:], onest_d[:])
            nc.sync.dma_start(onesr_t[:], onesr_d[:])
            for j in range(XK):
                nc.sync.dma_start(wts[j][:], wt[j][:])

            # per-edge src norm: w = rsqrt(deg_out[src]) (pad slots carry 1.0);
            # two pieces so the first masks aren't gated on the full tile
            for c0, c1 in ((0, dsplit), (dsplit, n_mm)):
                if c0 >= c1:
                    continue
                nc.vector.reciprocal(wsrc_t[:, c0:c1], wsrc_t[:, c0:c1])
                nc.scalar.activation(
                    wsrc_t[:, c0:c1], wsrc_t[:, c0:c1],
                    mybir.ActivationFunctionType.Sqrt,
                )
            # dst norm: rsqrt(max(deg_in, 1)) tile-major [GRP, NG]
            nc.vector.tensor_scalar(
                ndst_t[:], degi_t[:], 1.0, None, op0=mybir.AluOpType.max
            )
            nc.vector.reciprocal(ndst_t[:], ndst_t[:])
            nc.scalar.activation(
                ndst_t[:], ndst_t[:], mybir.ActivationFunctionType.Sqrt
            )

            stats_in = dpool.tile([1, 2 * OUT], F32)
            _aspace = "Local" if cfg.get("NOCC") else "Shared"
            stats_out = dpool.tile([C, 2 * OUT], F32, addr_space=_aspace)

            hrelu_t = hpool.tile([128, NG, OUT], BF16)

            # ---- PSUM layout (8 banks x 2KB); accumulation-group zeroing is
            # bank-granular, so every concurrently-open chain gets its own
            # bank: 4x agg (GCHUNK groups in flight) + 1x transpose + 2x out
            # (alternating, WAR-tracked) + 1x BN stats (sum+sq as one chain).
            assert GC in (4, 5)
            # agg tiles carry a 512B tail used (bitcast bf16) as the per-group
            # transpose target, so each group's finish chain owns its own bank
            ps_agg = [
                pagg.tile([128, 2 * OUT + 128], F32, name=f"ps_agg{i}")
                for i in range(GC)
            ]
            n_out = 8 - GC - 1
            ps_out = [
                paux.tile([128, OUT], F32, name=f"ps_out{i}") for i in range(n_out)
            ]
            ps_stat = pspool.tile([1, 2, OUT], F32, name="ps_stat")

            # ---- stage D: gather x rows + one-hot matmul segmented sum ----
            def _finish_group(g):
                """Group g's PSUM agg is complete: apply W, relu, BN partials."""
                gi = g % GC
                cagg = capool.tile([128, 2 * OUT], BF16, tag="cagg")
                nc.scalar.activation(
                    cagg[:], ps_agg[gi][:, : 2 * OUT],
                    mybir.ActivationFunctionType.Copy,
                )
                # transpose into this group's own agg bank tail (bitcast bf16)
                # so concurrent groups' finish chains don't share a PSUM bank
                trv = ps_agg[gi][:, 2 * OUT :].bitcast(BF16)
                for h in range(2):
                    nc.tensor.matmul(
                        trv[:, h * OUT : (h + 1) * OUT],
                        cagg[:, h * OUT : (h + 1) * OUT],
                        ident_t[:],
                        is_transpose=True,
                        start=(h == 0),
                        stop=(h == 1),
                    )
                tagg = capool.tile([128, 2, OUT], BF16, tag="tagg")
                nc.scalar.activation(
                    tagg[:, :, :],
                    trv[:].rearrange("p (a f) -> p a f", a=2),
                    mybir.ActivationFunctionType.Copy,
                )
                po = ps_out[g % len(ps_out)]
                for jj in range(XK):
                    nc.tensor.matmul(
                        po[:],
                        tagg[:, jj, :],
                        wts[jj][:],
                        start=(jj == 0),
                        stop=(jj == XK - 1),
                    )
                # stage E: relu(out * ndst + b), BN partial sums
                tmp = epool.tile([128, OUT], F32, tag="etmp")
                nc.vector.scalar_tensor_tensor(
                    tmp[:],
                    po[:],
                    ndst_t[:, g : g + 1],
                    bt_t[:],
                    op0=mybir.AluOpType.mult,
                    op1=mybir.AluOpType.add,
                )
                nc.scalar.activation(
                    hrelu_t[:, g, :], tmp[:], mybir.ActivationFunctionType.Relu
                )
                ones = onesc_t if g < NG - 1 else onest_t
                nc.tensor.matmul(
                    ps_stat[:, 0, :],
                    ones[:],
                    hrelu_t[:, g, :],
                    start=(g == 0),
                    stop=False,
                )
                sq = epool.tile([128, OUT], BF16, tag="esq")
                nc.scalar.activation(
                    sq[:], hrelu_t[:, g, :], mybir.ActivationFunctionType.Square
                )
                nc.tensor.matmul(
                    ps_stat[:, 1, :],
                    ones[:],
                    sq[:],
                    start=False,
                    stop=(g == NG - 1),
                )

            bmax = max(nb for _, _, nb in batches)
            # finish chains are deferred until the group's PSUM slot is about
            # to be reused, so they sit behind the next chunk's early mask
            # matmuls in the in-order PE queue instead of blocking them
            pending = {}
            for bank, t0, nblk in batches:
                Gt = gpool.tile([128, bmax, IN], BF16, tag="G")
                nc.gpsimd.dma_gather(
                    Gt[:, :nblk, :],
                    xb[bank][:],
                    gidx_t[:, t0 * 8 : (t0 + nblk) * 8],
                    nblk * 128,
                    nblk * 128,
                    IN,
                    single_packet=False,
                )
                for j in range(nblk):
                    t = t0 + j
                    for m, g in block_mms[t]:
                      is_start, is_stop = mm_flags[m]
                      gi = g % GC
                      if is_start and gi in pending:
                          _finish_group(pending.pop(gi))
                      Mt = mpool.tile([128, GRP], BF16, tag="M")
                      nc.vector.tensor_scalar(
                          Mt[:],
                          iota_t[:],
                          doff_t[:, m : m + 1],
                          wsrc_t[:, m : m + 1],
                          op0=mybir.AluOpType.is_equal,
                          op1=mybir.AluOpType.mult,
                      )
                      nc.tensor.matmul(
                          ps_agg[gi][:, : 2 * OUT],
                          Mt[:],
                          Gt[:, j, :],
                          start=is_start,
                          stop=is_stop,
                      )
                      if is_stop:
                          pending[gi] = g

            for gi in sorted(pending):
                _finish_group(pending.pop(gi))

            # ---- stage F: AllReduce BN stats; build affine S/T tiles ----
            S_t = cpool.tile([128, OUT], BF16)
            T_t = cpool.tile([128, OUT], BF16)
            st_sb = cpool.tile([1, 2 * OUT], F32)
            nc.scalar.activation(
                st_sb[:], ps_stat[:].rearrange("p a f -> p (a f)"),
                mybir.ActivationFunctionType.Copy,
            )
            nc.sync.dma_start(stats_in[:], st_sb[:])
            if cfg.get("NOCC"):
                stats_out = stats_in  # single-core debug: sums are the totals
                st8 = st_sb
                ones8v = None
            else:
                # AllGather (no 1.875x reduce multiplier) + tiny local
                # ones-matmul reduction beats AllReduce on latency
                nc.gpsimd.collective_compute(
                    "AllGather",
                    mybir.AluOpType.bypass,
                    replica_groups=[list(range(C))],
                    ins=[stats_in[:]],
                    outs=[stats_out[:]],
                )
                st8 = cpool.tile([C, 2 * OUT], F32)
                nc.sync.dma_start(st8[:], stats_out[:])
                ones8v = ones8_t
            if ones8v is None:
                st_rb = cpool.tile([1, 2 * OUT], F32)
                nc.scalar.activation(
                    st_rb[:], st8[:], mybir.ActivationFunctionType.Copy
                )
                red_v = st_rb[:]
            else:
                ps_red = ps_stat[:].rearrange("p a f -> p (a f)")
                nc.tensor.matmul(ps_red, ones8v[:], st8[:], start=True, stop=True)
                red_v = ps_red

            mu = cpool.tile([1, OUT], F32)
            musq = cpool.tile([1, OUT], F32)
            ex2e = cpool.tile([1, OUT], F32)
            var = cpool.tile([1, OUT], F32)
            srow = cpool.tile([1, OUT], F32)
            trow = cpool.tile([1, OUT], F32)
            inv_n = 1.0 / float(N)
            # mu/ex2 straight off the reduction PSUM; fold 1/N and eps in
            nc.scalar.activation(
                mu[:], red_v[:, 0:OUT], mybir.ActivationFunctionType.Copy, scale=inv_n
            )
            nc.scalar.activation(
                ex2e[:],
                red_v[:, OUT : 2 * OUT],
                mybir.ActivationFunctionType.Copy,
                scale=inv_n,
                bias=float(cfg["EPS"]),
            )
            nc.scalar.activation(
                musq[:], mu[:], mybir.ActivationFunctionType.Square
            )
            # var + eps, then rsqrt via reciprocal+sqrt (ACT Rsqrt is banned
            # for accuracy)
            nc.vector.tensor_sub(var[:], ex2e[:], musq[:])
            nc.vector.reciprocal(var[:], var[:])
            nc.scalar.activation(
                var[:], var[:], mybir.ActivationFunctionType.Sqrt
            )
            nc.vector.tensor_mul(srow[:], gm_t[:], var[:])
            nc.vector.tensor_mul(trow[:], mu[:], srow[:])
            nc.vector.tensor_sub(trow[:], bb_t[:], trow[:])

            # reuse the (now idle) out banks for the S/T broadcast matmuls
            nc.tensor.matmul(ps_out[0][:], onesr_t[:], srow[:], start=True, stop=True)
            nc.scalar.activation(
                S_t[:], ps_out[0][:], mybir.ActivationFunctionType.Copy
            )
            nc.tensor.matmul(ps_out[1][:], onesr_t[:], trow[:], start=True, stop=True)
            nc.scalar.activation(
                T_t[:], ps_out[1][:], mybir.ActivationFunctionType.Copy
            )

            # ---- stage G: y = hrelu * S + T (in place, S/T broadcast along
            # the group axis), output DMA chunked to overlap with the DVE ----
            ypad_view = ypad_d[:].rearrange("p (g f) -> p g f", g=NG)
            GOUT = 7
            for ci, c0 in enumerate(range(0, NG, GOUT)):
                c1 = min(c0 + GOUT, NG)
                S_b = S_t[:].rearrange("p (a f) -> p a f", a=1).to_broadcast(
                    (128, c1 - c0, OUT)
                )
                T_b = T_t[:].rearrange("p (a f) -> p a f", a=1).to_broadcast(
                    (128, c1 - c0, OUT)
                )
                eng = nc.gpsimd if ci >= 12 else nc.vector
                eng.tensor_mul(
                    hrelu_t[:, c0:c1, :], hrelu_t[:, c0:c1, :], S_b
                )
                eng.tensor_add(
                    hrelu_t[:, c0:c1, :], hrelu_t[:, c0:c1, :], T_b
                )
                nc.sync.dma_start(
                    ypad_view[:, c0:c1, :], hrelu_t[:, c0:c1, :]
                )

    nc.compile()
    return nc


def kernel(x, src, dst, W, b, gamma, beta):
    global LAST_RESULTS
    cfg = CFG
    N, E, IN, OUT, C = cfg["N"], cfg["E"], cfg["IN"], cfg["OUT"], cfg["NCORES"]
    GRP = cfg["GRP"]
    assert x.shape == (N, IN) and W.shape == (IN, OUT)
    assert src.shape == (E,) and dst.shape == (E,)

    meta, gidx_cores, dstoff_cores, wdeg_cores = _preprocess(cfg, src, dst)
    NPC, NG, BROWS = meta["NPC"], meta["NG"], meta["BROWS"]
    XK = _ceil_div(IN, 128)
    last_w = NPC - (NG - 1) * GRP

    nc = _build_nc(cfg, meta)

    import ml_dtypes

    x_bf = np.asarray(x, np.float32).astype(ml_dtypes.bfloat16)
    Wn = np.asarray(W, np.float32)

    iota = np.tile(
        np.arange(GRP, dtype=np.float32)[None, :], (128, 1)
    ).astype(ml_dtypes.bfloat16)
    ident = np.eye(128, dtype=np.float32).astype(ml_dtypes.bfloat16)
    bt = np.tile(np.asarray(b, np.float32)[None, :], (128, 1))
    onesc = np.ones((128, 1), np.float32).astype(ml_dtypes.bfloat16)
    ones8 = np.ones((8, 1), np.float32)
    onest = np.zeros((128, 1), np.float32)
    onest[:last_w] = 1.0
    onest = onest.astype(ml_dtypes.bfloat16)
    onesr = np.ones((1, 128), np.float32)
    gm = np.asarray(gamma, np.float32)[None, :]
    bb = np.asarray(beta, np.float32)[None, :]
    xbanks = [
        np.ascontiguousarray(x_bf[q * BROWS : (q + 1) * BROWS])
        for q in range(cfg["NBANKS"])
    ]
    wtiles = [
        np.ascontiguousarray(Wn[j * 128 : (j + 1) * 128, :]).astype(
            ml_dtypes.bfloat16
        )
        for j in range(XK)
    ]

    in_maps = []
    for k in range(C):
        im = {
            "gidx": gidx_cores[k],
            "doff": dstoff_cores[k],
            "wdeg": wdeg_cores[k],
            "degi": _tile_major(
                meta["deg_in"][k * NPC : (k + 1) * NPC], NG, GRP, np.float32(1.0)
            ),
            "bt": bt,
            "iota": iota,
            "ident": ident,
            "gm": gm,
            "bb": bb,
            "onesc": onesc,
            "ones8": ones8,
            "onest": onest,
            "onesr": onesr,
        }
        for q in range(cfg["NBANKS"]):
            im[f"xb{q}"] = xbanks[q]
        for j in range(XK):
            im[f"wt{j}"] = wtiles[j]
        in_maps.append(im)

    if cfg.get("SIM"):
        from concourse.bass_interp import MultiCoreSim

        sim = MultiCoreSim(nc, num_cores=C)
        for k, core_sim in sim.cores.items():
            for name, val in in_maps[k].items():
                core_sim.tensor(name)[:] = val
        sim.simulate()
        y = np.empty((N, OUT), np.float32)
        perm = meta["perm"]
        for k in range(C):
            arr = np.asarray(sim.cores[k].tensor("ypad"), dtype=np.float32)
            arr = arr.reshape(128, NG, OUT).transpose(1, 0, 2).reshape(-1, OUT)
            y[perm[k * NPC : (k + 1) * NPC]] = arr[:NPC]
        return y

    global LAST_NC, LAST_RUN_S
    LAST_NC = nc
    import time as _time

    _t0 = _time.time()
    res = bass_utils.run_bass_kernel_spmd(
        nc,
        in_maps,
        core_ids=list(range(C)),
        trace=cfg.get("TRACE", False),
    )
    LAST_RUN_S = _time.time() - _t0
    LAST_RESULTS = res

    y = np.empty((N, OUT), np.float32)
    perm = meta["perm"]
    for k in range(C):
        arr = np.asarray(res.results[k]["ypad"], dtype=np.float32)
        arr = arr.reshape(128, NG, OUT).transpose(1, 0, 2).reshape(-1, OUT)
        y[perm[k * NPC : (k + 1) * NPC]] = arr[:NPC]
    return y
